# revision 4
# baseline (speedup 1.0000x reference)
"""DNC MemoryAccess kernel for Trainium2 (Bass/Tile), data-parallel over batch.

Shapes (hardcoded): B=8, T=16, C=1024, IFACE=471, N=512, WORD=64, R=4, NW=1.
Each of the 8 cores processes one batch element; all recurrent state stays
SBUF-resident across the T=16 sequential steps.

vs the original implementation: the backward temporal-link matrix LT is
maintained by its own elementwise recurrence (no per-step PE transposes), the
link diagonal is zeroed by subtracting an ident-scaled rank-1 fix, read
weights are assembled in one PSUM accumulation (mode-scaled bwd + fwd +
content via a diagonal stationary), the allocation chain for step t+1 is
software-pipelined into step t, read words are produced transposed ([W,R])
and fixed up by the output DMA, and engine assignment balances DVE / Pool /
Activation / PE under the hardware constraints that GPSIMD cannot touch PSUM
and only implements memset/add/multiply kernels.

Precision: table Exp/Ln only on smooth softmax/log paths (same error class as
the fp32 reference on this backend); usage comparisons, quake rsqrt and the
bit-trick mantissa log are exact DVE arithmetic, so the allocation sort
matches the reference except for genuine fp32 ties (b=7 carries one, same as
the reference implementation's own fp32 floor).
"""
import sys

sys.path.insert(0, "/opt/trn_rl_repo")

import numpy as np

import concourse.bacc as bacc
import concourse.bass as bass
import concourse.mybir as mybir
import concourse.tile as tile

F32 = mybir.dt.float32
F32R = mybir.dt.float32r
I32 = mybir.dt.int32
LN2 = 0.6931471805599453
AF = mybir.ActivationFunctionType
OP = mybir.AluOpType

B, T, C, IF = 8, 16, 1024, 471
N, W, R = 512, 64, 4
NT = N // 128

O_RK, O_RS, O_WK, O_WS = 0, 256, 260, 324
O_ER, O_WV, O_FG, O_AG, O_WG, O_MD = 325, 389, 453, 457, 458, 459


def fr(ap):
    return ap.bitcast(F32R)


def build_nc():
    nc = bacc.Bacc("TRN2", target_bir_lowering=False, debug=False, num_devices=8)

    co_d = nc.declare_dram_parameter("co", [T, C], F32, isOutput=False)
    w_d = nc.declare_dram_parameter("wif", [C, IF], F32, isOutput=False)
    b_d = nc.declare_dram_parameter("bif", [1, IF], F32, isOutput=False)
    m0_d = nc.declare_dram_parameter("mem0", [N, W], F32, isOutput=False)
    ident_d = nc.declare_dram_parameter("ident", [128, 128], F32, isOutput=False)
    out_d = nc.declare_dram_parameter("out", [T, R, W], F32, isOutput=True)

    with tile.TileContext(nc) as tc:
        with (
            nc.allow_low_precision(reason="float32r tiles are bit-identical fp32"),
            tc.tile_pool(name="const", bufs=1) as cp,
            tc.tile_pool(name="state", bufs=2) as sp,
            tc.tile_pool(name="work", bufs=2) as wp,
            tc.tile_pool(name="psB", bufs=4, space="PSUM") as psB,
            tc.tile_pool(name="psM", bufs=2, space="PSUM") as psM,
            tc.tile_pool(name="psS", bufs=2, space="PSUM") as psS,
        ):
            _build_body(nc, tc, cp, sp, wp, psB, psM, psS,
                        co_d, w_d, b_d, m0_d, ident_d, out_d)
    nc.compile()
    return nc



def _lnpm(nc, wp, x_ap, tagp):
    """ln(x) for PM tiles [128, NT], x > 0, via exponent split + atanh series.
    ~1e-7 relative; saturates gracefully at +-88 for denormal/zero input."""
    V, G_ = nc.vector, nc.gpsimd
    xi = x_ap.bitcast(I32)
    sh = wp.tile([128, NT], I32, tag=tagp + "sh")
    V.tensor_scalar(sh[:], xi, 23, None, op0=OP.arith_shift_right)
    eb = wp.tile([128, NT], I32, tag=tagp + "eb")
    V.tensor_scalar(eb[:], sh[:], -127, None, op0=OP.add)
    ef = wp.tile([128, NT], F32, tag=tagp + "ef")
    V.tensor_copy(ef[:], eb[:])
    efs = wp.tile([128, NT], F32, tag=tagp + "efs")
    V.tensor_scalar(efs[:], ef[:], LN2, None, op0=OP.mult)
    mi = wp.tile([128, NT], I32, tag=tagp + "mi")
    V.tensor_scalar(mi[:], xi, 0x007FFFFF, None, op0=OP.bitwise_and)
    mo = wp.tile([128, NT], I32, tag=tagp + "mo")
    V.tensor_scalar(mo[:], mi[:], 0x3F800000, None, op0=OP.bitwise_or)
    m_ = mo[:].bitcast(F32)
    nm = wp.tile([128, NT], F32, tag=tagp + "nm")
    V.tensor_scalar(nm[:], m_, -1.0, None, op0=OP.add)
    dn = wp.tile([128, NT], F32, tag=tagp + "dn")
    V.tensor_scalar(dn[:], m_, 1.0, None, op0=OP.add)
    di = wp.tile([128, NT], F32, tag=tagp + "di")
    V.reciprocal(di[:], dn[:])
    t_ = wp.tile([128, NT], F32, tag=tagp + "t")
    G_.tensor_tensor(t_[:], nm[:], di[:], op=OP.mult)
    t2 = wp.tile([128, NT], F32, tag=tagp + "t2")
    V.tensor_scalar(t2[:], t_[:], 2.0, None, op0=OP.mult)
    y_ = wp.tile([128, NT], F32, tag=tagp + "y")
    G_.tensor_tensor(y_[:], t_[:], t_[:], op=OP.mult)
    a_ = wp.tile([128, NT], F32, tag=tagp + "a")
    V.tensor_scalar(a_[:], y_[:], 1.0 / 9.0, None, op0=OP.mult)
    b_ = wp.tile([128, NT], F32, tag=tagp + "b")
    V.scalar_tensor_tensor(b_[:], a_[:], 1.0 / 7.0, y_[:], op0=OP.add,
                           op1=OP.mult)
    V.scalar_tensor_tensor(a_[:], b_[:], 1.0 / 5.0, y_[:], op0=OP.add,
                           op1=OP.mult)
    V.scalar_tensor_tensor(b_[:], a_[:], 1.0 / 3.0, y_[:], op0=OP.add,
                           op1=OP.mult)
    lm = wp.tile([128, NT], F32, tag=tagp + "lm")
    V.scalar_tensor_tensor(lm[:], b_[:], 1.0, t2[:], op0=OP.add, op1=OP.mult)
    out = wp.tile([128, NT], F32, tag=tagp + "o")
    V.tensor_tensor(out[:], lm[:], efs[:], op=OP.add)
    return out



def _alloc_early(nc, wp, psB, ones, u_pm_n, u_fl_n):
    """lnu + usage broadcast + comparison matrix for the NEXT step."""
    V, S, P, G_ = nc.vector, nc.scalar, nc.tensor, nc.gpsimd
    lnu = _lnpm(nc, wp, u_pm_n[:], "lu_")
    ub_p = psB.tile([128, N], F32, tag="b")
    P.matmul(ub_p[:], ones[0:1, :], u_fl_n[:])
    ubs = wp.tile([128, N], F32, tag="ubs")
    S.copy(ubs[:], ub_p[:])
    Gt = wp.tile([128, NT, N], F32, tag="G")
    for c in range(NT):
        eng = V if c < 2 else G_
        eng.tensor_scalar(Gt[:, c, :], ubs[:], u_pm_n[:, c:c + 1], None,
                          op0=OP.is_gt)
    return lnu, Gt


def _alloc_late(nc, wp, psM, lnu, Gt, u_fl_n):
    """log-sum matmul + exp + (u-1)*alloc for the NEXT step."""
    V, S, P, G_ = nc.vector, nc.scalar, nc.tensor, nc.gpsimd
    s_p = psM.tile([1, N], F32, tag="m")
    for c in range(NT):
        P.matmul(s_p[:], lnu[:, c:c + 1], Gt[:, c, :],
                 start=(c == 0), stop=(c == NT - 1))
    es = wp.tile([1, N], F32, tag="es")
    S.activation(es[:], s_p[:], AF.Exp)
    negalloc = wp.tile([1, N], F32, tag="na")
    nsa = wp.tile([1, 1], F32, tag="nsa")
    V.scalar_tensor_tensor(negalloc[:], u_fl_n[:], 1.0, es[:],
                           op0=OP.subtract, op1=OP.mult, accum_out=nsa[:])
    return negalloc, nsa


def _build_body(nc, tc, cp, sp, wp, psB, psM, psS,
                co_d, w_d, b_d, m0_d, ident_d, out_d):
    V, S, P, G_, DMA = nc.vector, nc.scalar, nc.tensor, nc.gpsimd, nc.sync

    # ---------------- constants ----------------
    ident = cp.tile([128, 128], F32)
    DMA.dma_start(ident[:], ident_d[:])
    ones = cp.tile([128, 128], F32)
    G_.memset(ones[:], 1.0)

    # persistent per-t tables
    iface = cp.tile([T, IF], F32)          # raw iface rows
    wvR = cp.tile([1, T, W], F32)          # write vectors, partition 0 rows
    keysc = cp.tile([W, 5, T], F32)        # scaled keys: r=0..3 read, 4 write
    neg_er = cp.tile([W, T], F32)
    gr = cp.tile([1, 6, T], F32)           # sigmoids: fg x4, ag, wg
    c1p = cp.tile([1, T], F32)
    cn1 = cp.tile([1, T], F32)
    c2 = cp.tile([1, T], F32)
    modes1 = cp.tile([1, R, T], F32)       # content-mode row per t
    mbs0 = cp.tile([128, R, T], F32)
    mbs2 = cp.tile([128, R, T], F32)
    nege0_pm = cp.tile([128, NT], F32)
    G_.memset(nege0_pm[:], 0.0)
    G_.memset(nege0_pm[0:1, 0:1], -1.0)
    out_sb = cp.tile([W, T, R], F32)

    # ---------------- prologue ----------------
    with tc.tile_pool(name="prolog", bufs=1) as pp:
        co_sb = pp.tile([T, C], F32)
        DMA.dma_start(co_sb[:], co_d[:])
        bif_sb = pp.tile([1, IF], F32)
        DMA.dma_start(bif_sb[:], b_d[:])
        w_sb = pp.tile([128, 8, IF], F32)
        for k in range(8):
            DMA.dma_start(w_sb[:, k, :], w_d[128 * k:128 * (k + 1), :])

        coT_p = psB.tile([128, 8, T], F32, tag="b")
        for k in range(8):
            P.transpose(coT_p[:, k, :], co_sb[:, 128 * k:128 * (k + 1)],
                        ident[0:T, 0:T])
        coT = pp.tile([128, 8, T], F32)
        V.tensor_copy(coT[:], coT_p[:])

        if_p = psM.tile([T, IF], F32, tag="m")
        for k in range(8):
            P.matmul(if_p[:], coT[:, k, :], w_sb[:, k, :],
                     start=(k == 0), stop=False)
        P.matmul(if_p[:], ones[0:1, 0:T], bif_sb[:],
                 start=False, stop=True)
        V.tensor_copy(iface[:], if_p[:])

        # keys [64, 5, T]: read r=0..3, write at 4
        keys_p = psB.tile([W, 5, T], F32, tag="b")
        for r in range(R):
            P.transpose(keys_p[:, r, :], iface[:, O_RK + W * r:O_RK + W * (r + 1)],
                        ident[0:T, 0:T])
        P.transpose(keys_p[:, 4, :], iface[:, O_WK:O_WK + W], ident[0:T, 0:T])
        keys = pp.tile([W, 5, T], F32)
        V.tensor_copy(keys[:], keys_p[:])

        # write vectors as partition-0 rows via selector matmuls
        for tt_ in range(T):
            wv_p = psS.tile([1, W], F32, tag="s", name=f"wvr{tt_}")
            P.matmul(wv_p[:], ident[0:T, tt_:tt_ + 1],
                     iface[:, O_WV:O_WV + W])
            V.tensor_copy(wvR[0:1, tt_, :], wv_p[:])

        # erase sigmoid -> neg_er
        er_p = psS.tile([W, T], F32, tag="s")
        P.transpose(er_p[:], iface[:, O_ER:O_ER + W], ident[0:T, 0:T])
        ee = pp.tile([W, T], F32)
        S.activation(ee[:], er_p[:], AF.Exp, scale=-1.0)
        ew = pp.tile([W, T], F32)
        V.tensor_scalar(ew[:], ee[:], 1.0, None, op0=OP.add)
        er_r = pp.tile([W, T], F32)
        V.reciprocal(er_r[:], ew[:])
        V.tensor_scalar(neg_er[:], er_r[:], -1.0, None, op0=OP.mult)

        # strengths softplus: [1, 5, T] (rs x4, ws)
        sts_p = psS.tile([1, 5, T], F32, tag="s")
        for r in range(R):
            P.transpose(sts_p[0:1, r, :], iface[:, O_RS + r:O_RS + r + 1],
                        ident[0:T, 0:T])
        P.transpose(sts_p[0:1, 4, :], iface[:, O_WS:O_WS + 1], ident[0:T, 0:T])
        st_e = pp.tile([1, 5 * T], F32)
        S.activation(st_e[:], sts_p[:].rearrange("o f t -> o (f t)"), AF.Exp)
        st_w = pp.tile([1, 5 * T], F32)
        V.tensor_scalar(st_w[:], st_e[:], 1.0, None, op0=OP.add)
        st_sp = pp.tile([1, 5 * T], F32)
        S.activation(st_sp[:], st_w[:], AF.Ln)

        # key norms: rsqrt(sum keys^2) = exp(-0.5 ln)
        sqk = pp.tile([W, 5 * T], F32)
        S.activation(sqk[:], keys[:].rearrange("w f t -> w (f t)"), AF.Square)
        k2_p = psM.tile([1, 5 * T], F32, tag="m")
        P.matmul(k2_p[:], ones[0:W, 0:1], sqk[:])
        lk2 = pp.tile([1, 5 * T], F32)
        S.activation(lk2[:], k2_p[:], AF.Ln)
        kr = pp.tile([1, 5 * T], F32)
        S.activation(kr[:], lk2[:], AF.Exp, scale=-0.5)
        beta = pp.tile([1, 5 * T], F32)
        V.tensor_tensor(beta[:], st_sp[:], kr[:], op=OP.mult)
        kb_p = psB.tile([W, 5 * T], F32, tag="b")
        P.matmul(kb_p[:], ones[0:1, 0:W], beta[:])
        V.tensor_tensor(keysc[:].rearrange("w f t -> w (f t)"),
                        keys[:].rearrange("w f t -> w (f t)"), kb_p[:],
                        op=OP.mult)

        # gates: fg x4, ag, wg sigmoids
        gats_p = psS.tile([1, 6, T], F32, tag="s")
        for r in range(R):
            P.transpose(gats_p[0:1, r, :], iface[:, O_FG + r:O_FG + r + 1],
                        ident[0:T, 0:T])
        P.transpose(gats_p[0:1, 4, :], iface[:, O_AG:O_AG + 1], ident[0:T, 0:T])
        P.transpose(gats_p[0:1, 5, :], iface[:, O_WG:O_WG + 1], ident[0:T, 0:T])
        g_e = pp.tile([1, 6 * T], F32)
        S.activation(g_e[:], gats_p[:].rearrange("o g t -> o (g t)"), AF.Exp,
                     scale=-1.0)
        g_w = pp.tile([1, 6 * T], F32)
        V.tensor_scalar(g_w[:], g_e[:], 1.0, None, op0=OP.add)
        V.reciprocal(gr[:].rearrange("o g t -> o (g t)"), g_w[:])
        ag_t = gr[0:1, 4, :]
        wg_t = gr[0:1, 5, :]
        V.tensor_tensor(c1p[:], ag_t, wg_t, op=OP.mult)
        V.tensor_scalar(cn1[:], c1p[:], -1.0, None, op0=OP.mult)
        V.tensor_tensor(c2[:], wg_t, c1p[:], op=OP.subtract)

        # modes softmax -> modes [12, T] rows 4m+r
        me = pp.tile([T, 12], F32)
        S.activation(me[:], iface[:, O_MD:O_MD + 12], AF.Exp)
        me3 = me[:].rearrange("t (r m) -> t r m", m=3)
        msum = pp.tile([T, R], F32)
        V.tensor_tensor(msum[:], me3[:, :, 0], me3[:, :, 1], op=OP.add)
        V.tensor_tensor(msum[:], msum[:], me3[:, :, 2], op=OP.add)
        mrcp = pp.tile([T, R], F32)
        V.reciprocal(mrcp[:], msum[:])
        mn = pp.tile([T, 12], F32)
        mn3 = mn[:].rearrange("t (m r) -> t m r", r=R)
        me3b = me[:].rearrange("t (r m) -> t m r", m=3)
        for m in range(3):
            V.tensor_tensor(mn3[:, m, :], me3b[:, m, :], mrcp[:], op=OP.mult)
        # three m-blocks at base partition 0: modes0/1/2 [4, T]
        mblk_p = psS.tile([R, 3, T], F32, tag="s")
        for m in range(3):
            P.transpose(mblk_p[:, m, :], mn[:, 4 * m:4 * (m + 1)],
                        ident[0:T, 0:T])
        mblk = pp.tile([R, 3, T], F32)
        V.tensor_copy(mblk[:], mblk_p[:])
        for r in range(R):
            s1_p = psS.tile([1, T], F32, tag="s", name=f"m1r{r}")
            P.matmul(s1_p[:], ident[0:R, r:r + 1], mblk[:, 1, :])
            V.tensor_copy(modes1[0:1, r, :], s1_p[:])
        # flatten rows r of m-block 0/2 onto partition 0 via selector matmuls
        mrows = pp.tile([1, 2, R, T], F32)
        for r in range(R):
            s0_p = psS.tile([1, T], F32, tag="s", name=f"m0r{r}")
            P.matmul(s0_p[:], ident[0:R, r:r + 1], mblk[:, 0, :])
            V.tensor_copy(mrows[0:1, 0, r, :], s0_p[:])
            s2_p = psS.tile([1, T], F32, tag="s", name=f"m2r{r}")
            P.matmul(s2_p[:], ident[0:R, r:r + 1], mblk[:, 2, :])
            V.tensor_copy(mrows[0:1, 1, r, :], s2_p[:])
        mb0_p = psB.tile([128, R * T], F32, tag="b")
        P.matmul(mb0_p[:], ones[0:1, :], mrows[0:1, 0, :, :])
        V.tensor_copy(mbs0[:].rearrange("p r t -> p (r t)"), mb0_p[:])
        mb2_p = psB.tile([128, R * T], F32, tag="b")
        P.matmul(mb2_p[:], ones[0:1, :], mrows[0:1, 1, :, :])
        V.tensor_copy(mbs2[:].rearrange("p r t -> p (r t)"), mb2_p[:])

    # ---------------- initial state ----------------
    mem_nrm = sp.tile([128, NT, W], F32, tag="mem_nrm")
    for c in range(NT):
        DMA.dma_start(mem_nrm[:, c, :],
                      m0_d[128 * c:128 * (c + 1), :])
    memT_p = psB.tile([W, N], F32, tag="b")
    for c in range(NT):
        P.transpose(memT_p[:, 128 * c:128 * (c + 1)],
                    mem_nrm[:, c, :], ident[:])
    memT = sp.tile([W, N], F32, tag="memT")
    V.tensor_copy(memT[:], memT_p[:])

    # initial norm + memN
    sqm0 = wp.tile([W, N], F32, tag="sqm")
    G_.tensor_tensor(sqm0[:], memT[:], memT[:], op=OP.mult)
    msf_p0 = psM.tile([1, N], F32, tag="m")
    P.matmul(msf_p0[:], ones[0:W, 0:1], sqm0[:])
    lms0 = wp.tile([1, N], F32, tag="lms")
    S.activation(lms0[:], msf_p0[:], AF.Ln)
    mrs0 = wp.tile([1, N], F32, tag="mrs")
    S.activation(mrs0[:], lms0[:], AF.Exp, scale=-0.5)
    mn_tp0 = psS.tile([128, NT], F32, tag="s")
    for c in range(NT):
        P.transpose(mn_tp0[:, c:c + 1], mrs0[0:1, 128 * c:128 * (c + 1)],
                    ident[0:1, 0:1])
    mnorm_i = sp.tile([128, NT], F32, tag="mnorm")
    V.tensor_copy(mnorm_i[:], mn_tp0[:])


    L = sp.tile([128, NT, N], F32, tag="L")
    G_.memset(L[:], 0.0)
    LT0 = sp.tile([128, NT, N], F32, tag="LT")
    G_.memset(LT0[:], 0.0)
    u_pm0 = sp.tile([128, NT], F32, tag="u_pm")
    G_.memset(u_pm0[:], 0.0)
    u_fl0 = sp.tile([1, N], F32, tag="u_fl")
    G_.memset(u_fl0[:], 0.0)
    rwT0 = sp.tile([128, NT * R], F32, tag="rwT")
    G_.memset(rwT0[:], 0.0)

    st = dict(memT=memT, mem_nrm=mem_nrm, mnorm=mnorm_i, L=L, LT=LT0,
              u_pm=u_pm0, u_fl=u_fl0, prec_pm=None, prec_fl=None,
              rwT=rwT0)

    for t in range(T):
        st = _step(nc, t, st, cp, sp, wp, psB, psM, psS,
                   ident, ones, iface, wvR, keysc, neg_er, gr, c1p, cn1, c2,
                   modes1, mbs0, mbs2, nege0_pm, out_sb)

    DMA.dma_start(out_d[:].rearrange("t r w -> w t r"), out_sb[:])


def _step(nc, t, st, cp, sp, wp, psB, psM, psS,
          ident, ones, iface, wvR, keysc, neg_er, gr, c1p, cn1, c2,
          modes1, mbs0, mbs2, nege0_pm, out_sb):
    V, S, P, G_ = nc.vector, nc.scalar, nc.tensor, nc.gpsimd
    memT, mem_nrm, mnorm = st["memT"], st["mem_nrm"], st["mnorm"]
    L, LT, u_pm, u_fl = st["L"], st["LT"], st["u_pm"], st["u_fl"]
    prec_pm, prec_fl, rwT = st["prec_pm"], st["prec_fl"], st["rwT"]
    last = (t == T - 1)

    # ---- allocation inputs (computed during the previous step) ----
    if t == 0:
        na_pm, nsa = nege0_pm, None
    else:
        na_pm, nsa = st["na_pm"], st["nsa"]

    # ---- write content softmax (PM, fast mm-sum) ----
    wdots_p = psS.tile([128, NT], F32, tag="s")
    for b in range(NT):
        P.matmul(wdots_p[:, b:b + 1], memT[:, 128 * b:128 * (b + 1)],
                 keysc[:, 4, t:t + 1])
    wlog = wp.tile([128, NT], F32, tag="wlog")
    V.tensor_tensor(wlog[:], wdots_p[:], mnorm[:], op=OP.mult)
    wexp_pm = wp.tile([128, NT], F32, tag="wexp")
    S.activation(wexp_pm[:], wlog[:], AF.Exp)
    wps_p = psS.tile([1, NT], F32, tag="s")
    P.matmul(wps_p[:], ones[:, 0:1], wexp_pm[:])
    wsum = wp.tile([1, 1], F32, tag="wsum")
    V.tensor_reduce(wsum[:], wps_p[:], axis=mybir.AxisListType.X, op=OP.add)
    wrs = wp.tile([1, 1], F32, tag="wrs")
    V.reciprocal(wrs[:], wsum[:])
    cw = wp.tile([1, 1], F32, tag="cw")
    V.tensor_tensor(cw[:], wrs[:], c2[0:1, t:t + 1], op=OP.mult)

    # ---- ww assembly (PM) ----
    cn1b_p = psS.tile([128, 1], F32, tag="s")
    P.matmul(cn1b_p[:], ones[0:1, :], cn1[0:1, t:t + 1])
    cwb_p = psS.tile([128, 1], F32, tag="s")
    P.matmul(cwb_p[:], ones[0:1, :], cw[:])
    wwx = wp.tile([128, NT], F32, tag="wwx")
    V.tensor_scalar(wwx[:], na_pm[:], cn1b_p[:, 0:1], None, op0=OP.mult)
    ww_pm = wp.tile([128, NT], F32, tag="wwpm")
    V.scalar_tensor_tensor(ww_pm[:], wexp_pm[:], cwb_p[:, 0:1], wwx[:],
                           op0=OP.mult, op1=OP.add)
    omw_pm = wp.tile([128, NT], F32, tag="omw")
    V.tensor_scalar(omw_pm[:], ww_pm[:], -1.0, 1.0, op0=OP.mult, op1=OP.add)
    ww_tp = psS.tile([1, N], F32, tag="s")
    for c in range(NT):
        P.transpose(ww_tp[0:1, 128 * c:128 * (c + 1)], ww_pm[:, c:c + 1],
                    ident[:])
    ww_fl = wp.tile([1, N], F32, tag="wwfl")
    S.copy(ww_fl[:], ww_tp[:])

    # ---- mode-scaled read-weight copies (ready immediately) ----
    if t > 0:
        rwTm0 = wp.tile([128, NT * R], F32, tag="rwTm0")
        V.tensor_tensor(rwTm0[:].rearrange("p (c r) -> p c r", r=R),
                        rwT[:].rearrange("p (c r) -> p c r", r=R),
                        mbs0[:, None, :, t].broadcast_to([128, NT, R]),
                        op=OP.mult)
        rwTm2 = wp.tile([128, NT * R], F32, tag="rwTm2")
        V.tensor_tensor(rwTm2[:].rearrange("p (c r) -> p c r", r=R),
                        rwT[:].rearrange("p (c r) -> p c r", r=R),
                        mbs2[:, None, :, t].broadcast_to([128, NT, R]),
                        op=OP.mult)

    # ---- sw + prec update ----
    if not last:
        if t == 0:
            omsw = None
        else:
            swa = wp.tile([1, 1], F32, tag="swa")
            V.tensor_tensor(swa[:], nsa[:], cn1[0:1, t:t + 1], op=OP.mult)
            sw = wp.tile([1, 1], F32, tag="sw")
            V.tensor_tensor(sw[:], swa[:], c2[0:1, t:t + 1], op=OP.add)
            omsw = wp.tile([1, 1], F32, tag="omsw")
            V.tensor_scalar(omsw[:], sw[:], -1.0, 1.0, op0=OP.mult,
                            op1=OP.add)
    if last:
        prec_pm_n, prec_fl_n = prec_pm, prec_fl
    else:
        prec_pm_n = sp.tile([128, NT], F32, tag="prec_pm")
        if t == 0:
            V.tensor_copy(prec_pm_n[:], ww_pm[:])
        else:
            omsw_p = psS.tile([128, 1], F32, tag="s")
            P.matmul(omsw_p[:], ones[0:1, :], omsw[:])
            V.scalar_tensor_tensor(prec_pm_n[:], prec_pm[:], omsw_p[:, 0:1],
                                   ww_pm[:], op0=OP.mult, op1=OP.add)
        p4_tp = psS.tile([1, N], F32, tag="s")
        for c in range(NT):
            P.transpose(p4_tp[0:1, 128 * c:128 * (c + 1)],
                        prec_pm_n[:, c:c + 1], ident[:])
        prec_fl_n = sp.tile([1, N], F32, tag="prec_fl")
        S.copy(prec_fl_n[:], p4_tp[:])

    # ---- usage update ----
    if last:
        u_pm_n, u_fl_n = u_pm, u_fl
    else:
        u_pm_n = sp.tile([128, NT], F32, tag="u_pm")
        if t == 0:
            V.tensor_copy(u_pm_n[:], ww_pm[:])
        else:
            fgb_p = psS.tile([128, R], F32, tag="s")
            P.matmul(fgb_p[:], ones[0:1, :], gr[0:1, 0:R, t])
            yyT = wp.tile([128, NT, R], F32, tag="yyT")
            V.scalar_tensor_tensor(
                yyT[:], fgb_p[:, None, :].broadcast_to([128, NT, R]), -1.0,
                rwT[:].rearrange("p (c r) -> p c r", r=R),
                op0=OP.mult, op1=OP.mult)
            om = wp.tile([128, NT, R], F32, tag="om")
            V.tensor_scalar(om[:], yyT[:], 1.0, None, op0=OP.add)
            p1 = wp.tile([128, NT], F32, tag="p1")
            V.tensor_tensor(p1[:], om[:, :, 0], om[:, :, 1], op=OP.mult)
            p2 = wp.tile([128, NT], F32, tag="p2")
            V.tensor_tensor(p2[:], om[:, :, 2], om[:, :, 3], op=OP.mult)
            psi = wp.tile([128, NT], F32, tag="psi")
            V.tensor_tensor(psi[:], p1[:], p2[:], op=OP.mult)
            omu = wp.tile([128, NT], F32, tag="omu")
            V.tensor_scalar(omu[:], u_pm[:], -1.0, 1.0, op0=OP.mult,
                            op1=OP.add)
            tn = wp.tile([128, NT], F32, tag="tn")
            V.scalar_tensor_tensor(tn[:], ww_pm[:], 1.0, omu[:],
                                   op0=OP.subtract, op1=OP.mult)
            V.scalar_tensor_tensor(u_pm_n[:], tn[:], 1.0, psi[:],
                                   op0=OP.add, op1=OP.mult)
        u4_tp = psS.tile([1, N], F32, tag="s")
        for c in range(NT):
            P.transpose(u4_tp[0:1, 128 * c:128 * (c + 1)],
                        u_pm_n[:, c:c + 1], ident[:])
        u_fl_n = sp.tile([1, N], F32, tag="u_fl")
        S.copy(u_fl_n[:], u4_tp[:])

    # ---- memory head: broadcasts + keep (PE/ACT early) ----
    wwb_p = psB.tile([W, N], F32, tag="b")
    P.matmul(wwb_p[:], ones[0:1, 0:W], ww_fl[:])
    add_p = psB.tile([W, N], F32, tag="b")
    P.matmul(add_p[:], wvR[0:1, t, :], ww_fl[:])
    keep = wp.tile([W, N], F32, tag="keep")
    S.activation(keep[:], wwb_p[:], AF.Copy, scale=neg_er[:, t:t + 1],
                 bias=1.0)

    # ---- link update (elementwise, L and LT) ----
    if t == 0:
        L_n, LT_n = L, LT
    else:
        wb_p = psB.tile([128, N], F32, tag="b")
        P.matmul(wb_p[:], ones[0:1, :], ww_fl[:])
        pb_p = psB.tile([128, N], F32, tag="b")
        P.matmul(pb_p[:], ones[0:1, :], prec_fl[:])
        wbs = wp.tile([128, N], F32, tag="wbs")
        S.copy(wbs[:], wb_p[:])
        pbs = wp.tile([128, N], F32, tag="pbs")
        V.tensor_copy(pbs[:], pb_p[:])
        wp_pm = wp.tile([128, NT], F32, tag="wppm")
        V.tensor_tensor(wp_pm[:], ww_pm[:], prec_pm[:], op=OP.mult)
        L_n = sp.tile([128, NT, N], F32, tag="L")
        LT_n = sp.tile([128, NT, N], F32, tag="LT")
        for c in range(NT):
            eA, eB = G_, G_
            w1 = wp.tile([128, N], F32, tag=f"w1_{c % 2}")
            V.tensor_scalar(w1[:], wbs[:], omw_pm[:, c:c + 1], None,
                            op0=OP.subtract)
            p1 = wp.tile([128, N], F32, tag=f"p1_{c % 2}")
            V.tensor_scalar(p1[:], pbs[:], ww_pm[:, c:c + 1], None,
                            op0=OP.mult)
            p1T = wp.tile([128, N], F32, tag=f"p1T_{c % 2}")
            V.tensor_scalar(p1T[:], wbs[:], prec_pm[:, c:c + 1], None,
                            op0=OP.mult)
            t1 = wp.tile([128, N], F32, tag=f"t1_{c % 2}")
            eA.tensor_tensor(t1[:], w1[:], L[:, c, :], op=OP.mult)
            t1T = wp.tile([128, N], F32, tag=f"t1T_{c % 2}")
            eB.tensor_tensor(t1T[:], w1[:], LT[:, c, :], op=OP.mult)
            eB.tensor_tensor(L_n[:, c, :], p1[:], t1[:], op=OP.subtract)
            eA.tensor_tensor(LT_n[:, c, :], p1T[:], t1T[:], op=OP.subtract)
            fix = wp.tile([128, 128], F32, tag=f"fix_{c % 2}")
            S.activation(fix[:], ident[:], AF.Copy, scale=wp_pm[:, c:c + 1])
            blk = slice(128 * c, 128 * (c + 1))
            tl = wp.tile([128, 128], F32, tag=f"tl_{c % 2}")
            G_.tensor_tensor(tl[:], p1[:, blk], t1[:, blk], op=OP.subtract)
            G_.tensor_tensor(L_n[:, c, blk], tl[:], fix[:], op=OP.subtract)
            tlT = wp.tile([128, 128], F32, tag=f"tlT_{c % 2}")
            G_.tensor_tensor(tlT[:], p1T[:, blk], t1T[:, blk], op=OP.subtract)
            G_.tensor_tensor(LT_n[:, c, blk], tlT[:], fix[:], op=OP.subtract)

    # ---- memory rest ----
    m1 = wp.tile([W, N], F32, tag="m1")
    V.tensor_tensor(m1[:], memT[:], keep[:], op=OP.mult)
    memT_n = sp.tile([W, N], F32, tag="memT")
    V.tensor_tensor(memT_n[:], m1[:], add_p[:], op=OP.add)
    mem_nrm_p = psB.tile([128, NT, W], F32, tag="b")
    for c in range(NT):
        P.transpose(mem_nrm_p[:, c, :], memT_n[:, 128 * c:128 * (c + 1)],
                    ident[0:W, 0:W])
    mem_nrm_n = sp.tile([128, NT, W], F32, tag="mem_nrm")
    V.tensor_copy(mem_nrm_n[:], mem_nrm_p[:])
    sqn = wp.tile([128, NT, W], F32, tag="sqn")
    G_.tensor_tensor(sqn[:], mem_nrm_n[:], mem_nrm_n[:], op=OP.mult)

    # ---- alloc part 1 for next step (lnu + compare) ----
    if last:
        lnu_n, Gt_n = None, None
    else:
        lnu_n = _lnpm(nc, wp, u_pm_n[:], "lu_")
        ub_p = psB.tile([128, N], F32, tag="b")
        P.matmul(ub_p[:], ones[0:1, :], u_fl_n[:])
        ubs = wp.tile([128, N], F32, tag="ubs")
        S.copy(ubs[:], ub_p[:])
        Gt_n = wp.tile([128, NT, N], F32, tag="G", bufs=1)
        for c in range(NT):
            V.tensor_scalar(Gt_n[:, c, :], ubs[:], u_pm_n[:, c:c + 1],
                            None, op0=OP.is_gt)

    # ---- norm chain ----
    msq = wp.tile([128, NT], F32, tag="msq")
    V.tensor_reduce(msq[:], sqn[:], axis=mybir.AxisListType.X, op=OP.add)
    q_sh = wp.tile([128, NT], I32, tag="q_sh")
    V.tensor_scalar(q_sh[:], msq[:].bitcast(I32), 1, None,
                    op0=OP.arith_shift_right)
    q_nb = wp.tile([128, NT], I32, tag="q_nb")
    V.tensor_scalar(q_nb[:], q_sh[:], -1, None, op0=OP.bitwise_xor)
    q_y = wp.tile([128, NT], F32, tag="q_y")
    V.tensor_scalar(q_y[:].bitcast(I32), q_nb[:], 0x5F3759E0, None,
                    op0=OP.add)
    for qi in range(2):
        q_s = wp.tile([128, NT], F32, tag=f"q_s{qi}")
        V.tensor_tensor(q_s[:], q_y[:], q_y[:], op=OP.mult)
        q_t = wp.tile([128, NT], F32, tag=f"q_t{qi}")
        V.tensor_tensor(q_t[:], msq[:], q_s[:], op=OP.mult)
        q_h = wp.tile([128, NT], F32, tag=f"q_h{qi}")
        V.tensor_scalar(q_h[:], q_t[:], -0.5, 1.5, op0=OP.mult, op1=OP.add)
        q_y2 = wp.tile([128, NT], F32, tag=f"q_y2{qi}")
        V.tensor_tensor(q_y2[:], q_y[:], q_h[:], op=OP.mult)
        q_y = q_y2
    mnorm_n = sp.tile([128, NT], F32, tag="mnorm")
    V.tensor_copy(mnorm_n[:], q_y[:])

    # ---- bwd/fwd accumulation ----
    if t > 0:
        rwTm0 = wp.tile([128, NT * R], F32, tag="rwTm0")
        V.tensor_tensor(rwTm0[:].rearrange("p (c r) -> p c r", r=R),
                        rwT[:].rearrange("p (c r) -> p c r", r=R),
                        mbs0[:, None, :, t].broadcast_to([128, NT, R]),
                        op=OP.mult)
        rwTm2 = wp.tile([128, NT * R], F32, tag="rwTm2")
        V.tensor_tensor(rwTm2[:].rearrange("p (c r) -> p c r", r=R),
                        rwT[:].rearrange("p (c r) -> p c r", r=R),
                        mbs2[:, None, :, t].broadcast_to([128, NT, R]),
                        op=OP.mult)

    # ---- alloc part 2 for next step (PM via tiny block matmuls) ----
    if last:
        na_pm_n, nsa_n = None, None
    else:
        sT_p = psS.tile([128, NT], F32, tag="s")
        for b in range(NT):
            for c in range(NT):
                P.matmul(sT_p[:, b:b + 1], Gt_n[:, c, 128 * b:128 * (b + 1)],
                         lnu_n[:, c:c + 1], start=(c == 0),
                         stop=(c == NT - 1))
        es_pm = wp.tile([128, NT], F32, tag="espm")
        S.activation(es_pm[:], sT_p[:], AF.Exp)
        na_pm_n = wp.tile([128, NT], F32, tag="napm")
        nap = wp.tile([128, 1], F32, tag="nap")
        V.scalar_tensor_tensor(na_pm_n[:], u_pm_n[:], 1.0, es_pm[:],
                               op0=OP.subtract, op1=OP.mult,
                               accum_out=nap[:])
        nsp_p = psS.tile([1, 128], F32, tag="s")
        P.transpose(nsp_p[:], nap[:], ident[:])
        nsa_n = wp.tile([1, 1], F32, tag="nsa")
        V.tensor_reduce(nsa_n[:], nsp_p[:], axis=mybir.AxisListType.X,
                        op=OP.add)

    # ---- read content (PM via tiny block matmuls) ----
    rdots_p = psS.tile([128, NT * R], F32, tag="s")
    for b in range(NT):
        P.matmul(rdots_p[:, R * b:R * (b + 1)],
                 memT_n[:, 128 * b:128 * (b + 1)], keysc[:, 0:4, t])
    rlog = wp.tile([128, NT, R], F32, tag="rlog")
    V.tensor_tensor(rlog[:],
                    rdots_p[:].rearrange("p (c r) -> p c r", r=R),
                    mnorm_n[:, :, None].broadcast_to([128, NT, R]),
                    op=OP.mult)
    rexp_pm = wp.tile([128, NT * R], F32, tag="rexp")
    S.activation(rexp_pm[:], rlog[:].rearrange("p c r -> p (c r)"), AF.Exp)
    rps_p = psS.tile([1, NT * R], F32, tag="s")
    P.matmul(rps_p[:], ones[:, 0:1], rexp_pm[:])
    rsum = wp.tile([1, R], F32, tag="rsum")
    V.tensor_reduce(rsum[:], rps_p[:].rearrange("o (c r) -> o r c", r=R),
                    axis=mybir.AxisListType.X, op=OP.add)
    rsr = wp.tile([1, R], F32, tag="rsr")
    V.reciprocal(rsr[:], rsum[:])
    s1c = wp.tile([1, R], F32, tag="s1c")
    V.tensor_tensor(s1c[:], rsr[:], modes1[0:1, :, t], op=OP.mult)
    s1cb_p = psS.tile([128, R], F32, tag="s")
    P.matmul(s1cb_p[:], ones[0:1, :], s1c[:])

    # read weights TRANSPOSED in PM; content term fused into the PSUM->SBUF copy
    cnt = wp.tile([128, NT, R], F32, tag="cnt")
    V.tensor_tensor(cnt[:], rexp_pm[:].rearrange("p (c r) -> p c r", r=R),
                    s1cb_p[:, None, :].broadcast_to([128, NT, R]), op=OP.mult)
    rwT_n = sp.tile([128, NT * R], F32, tag="rwT")
    if t > 0:
        rwT_p = psS.tile([128, NT * R], F32, tag="s")
        for b in range(NT):
            blk = slice(128 * b, 128 * (b + 1))
            for c in range(NT):
                P.matmul(rwT_p[:, R * b:R * (b + 1)], L_n[:, c, blk],
                         rwTm0[:, R * c:R * (c + 1)],
                         start=(c == 0), stop=False)
            for c in range(NT):
                P.matmul(rwT_p[:, R * b:R * (b + 1)], LT_n[:, c, blk],
                         rwTm2[:, R * c:R * (c + 1)],
                         start=False, stop=(c == NT - 1))
        V.tensor_tensor(rwT_n[:], cnt[:].rearrange("p c r -> p (c r)"),
                        rwT_p[:], op=OP.add)
    else:
        V.tensor_copy(rwT_n[:], cnt[:].rearrange("p c r -> p (c r)"))

    # ---- read words (transposed output [W, R]) ----
    rwd_p = psS.tile([W, R], F32, tag="s")
    for c in range(NT):
        P.matmul(rwd_p[:], mem_nrm_n[:, c, :],
                 rwT_n[:, R * c:R * (c + 1)],
                 start=(c == 0), stop=(c == NT - 1))
    V.tensor_copy(out_sb[:, t, :], rwd_p[:])

    return dict(memT=memT_n, mem_nrm=mem_nrm_n, mnorm=mnorm_n, L=L_n,
                LT=LT_n, u_pm=u_pm_n, u_fl=u_fl_n, na_pm=na_pm_n,
                nsa=nsa_n, prec_pm=prec_pm_n, prec_fl=prec_fl_n, rwT=rwT_n)


# ---------------------------------------------------------------------------
_NC_CACHE = {}


def _get_nc():
    if "nc" not in _NC_CACHE:
        _NC_CACHE["nc"] = build_nc()
    return _NC_CACHE["nc"]


def _consts():
    ident = np.eye(128, dtype=np.float32)
    return (ident,)


def make_in_maps(controller_output, W_if, b_if, memory0):
    (ident,) = _consts()
    maps = []
    for b in range(B):
        maps.append({
            "co": np.ascontiguousarray(controller_output[b]),
            "wif": np.ascontiguousarray(W_if),
            "bif": np.ascontiguousarray(b_if.reshape(1, IF)),
            "mem0": np.ascontiguousarray(memory0[b]),
            "ident": ident,
        })
    return maps


def kernel(controller_output, W_if, b_if, memory0):
    from concourse.bass_utils import run_bass_kernel_spmd
    controller_output = np.asarray(controller_output, dtype=np.float32)
    W_if = np.asarray(W_if, dtype=np.float32)
    b_if = np.asarray(b_if, dtype=np.float32)
    memory0 = np.asarray(memory0, dtype=np.float32)
    nc = _get_nc()
    maps = make_in_maps(controller_output, W_if, b_if, memory0)
    res = run_bass_kernel_spmd(nc, maps, core_ids=list(range(B)))
    return np.stack([res.results[b]["out"] for b in range(B)], axis=0)


if __name__ == "__main__":
    mode = sys.argv[1] if len(sys.argv) > 1 else "sim"
    sys.path.insert(0, "/root/problem")
    import jax
    with jax.default_device(jax.devices("cpu")[0]):
        import reference
        inputs = {k: np.asarray(v) for k, v in reference.setup_inputs().items()}
        expected = np.asarray(reference.reference(**inputs))

    if mode == "sim":
        from concourse.bass_interp import CoreSim
        nc = build_nc()
        maps = make_in_maps(inputs["controller_output"], inputs["W_if"],
                            inputs["b_if"], inputs["memory0"])
        sim = CoreSim(nc)
        for k, v in maps[0].items():
            sim.tensor(k)[:] = v
        sim.simulate()
        got = sim.tensor("out").copy()
        exp = expected[0]
        err = np.abs(got - exp)
        rel = np.linalg.norm(got - exp) / (np.linalg.norm(exp) + 1e-12)
        print("sim modeled time (ns):", sim.time)
        print("max abs err:", err.max(), " rel err:", rel)
    else:
        got = kernel(**inputs)
        rel = np.linalg.norm(got - expected) / (np.linalg.norm(expected) + 1e-12)
        print("max abs err:", np.abs(got - expected).max(), " rel err:", rel)



# revision 7
# speedup vs baseline: 1.4619x; 1.4619x over previous
"""DNC MemoryAccess kernel for Trainium2 (Bass/Tile), data-parallel over batch.

Shapes (hardcoded): B=8, T=16, C=1024, IFACE=471, N=512, WORD=64, R=4, NW=1.
Each of the 8 cores processes one batch element; all recurrent state stays
SBUF-resident across the T=16 sequential steps.

v2 redesign vs the previous kernel:
- temporal link L and its transpose LT are kept in bf16; their elementwise
  recurrence uses shared w1 = (w_j - (1-w_i)) via fast-mode tensor_scalar
  (0.25x DVE cycles in bf16) plus tensor_tensor combines split across
  DVE/Pool,
- the link diagonal is never fixed up in-place: the scalar diagonal
  recurrence d = (1-2w)d + w*p is tracked separately ([128,NT]) and its
  contribution subtracted from the fwd/bwd matmul results,
- allocation ln(usage) and the memory-norm rsqrt use the Activation table
  Ln/Exp (one act-func set covers Exp/Ln/Copy/Square/Sign),
- broadcast matmuls and the iface GEMM run as float32r (1 cycle/row at
  >=256 free elems); the usage-compare broadcast stays exact fp32 so the
  allocation sort ties match the fp32 reference,
- u_fl/prec_fl PM->flat flattens are SBUF->SBUF DMAs (4 column descriptors
  each) issued on the SP queue: zero compute-engine cost,
- PSUM->SBUF copies land on the Activation engine, elementwise memory-update
  work on Pool, everything latency-critical stays on DVE.
"""
import sys

sys.path.insert(0, "/opt/trn_rl_repo")

import numpy as np

import concourse.bacc as bacc
import concourse.bass as bass
import concourse.mybir as mybir
import concourse.tile as tile

F32 = mybir.dt.float32
F32R = mybir.dt.float32r
BF16 = mybir.dt.bfloat16
I32 = mybir.dt.int32
AF = mybir.ActivationFunctionType
OP = mybir.AluOpType

B, T, C, IF = 8, 16, 1024, 471
N, W, R = 512, 64, 4
NT = N // 128

O_RK, O_RS, O_WK, O_WS = 0, 256, 260, 324
O_ER, O_WV, O_FG, O_AG, O_WG, O_MD = 325, 389, 453, 457, 458, 459


def fr(ap):
    return ap


# Prefer the activation-function set that contains Exp AND Ln (plus
# Copy/Square/Sign), so the per-step Exp/Ln mix resolves to one table and the
# compiler hoists a single LoadActFuncSet out of the step loop instead of
# thrashing 1283ns loads between exp-only and ln-only sets.
_ORIG_GET_ACT_TABLES = None


def _patch_act_tables():
    global _ORIG_GET_ACT_TABLES
    if _ORIG_GET_ACT_TABLES is not None:
        return
    import concourse.hw_specs as hw_specs
    _ORIG_GET_ACT_TABLES = hw_specs.get_activation_tables

    def pinned(arch):
        tabs = dict(_ORIG_GET_ACT_TABLES(arch))
        pref = "natural_log_exp_and_others"
        if pref not in tabs:
            return tabs
        exp_ln = {mybir.ActivationFunctionType.Exp,
                  mybir.ActivationFunctionType.Ln}
        out = {}
        for k, v in tabs.items():
            out[k] = set(v) if k == pref else set(v) - exp_ln
        return out

    bacc.get_activation_tables = pinned


def build_nc():
    _patch_act_tables()
    nc = bacc.Bacc("TRN2", target_bir_lowering=False, debug=False, num_devices=8)

    co_d = nc.declare_dram_parameter("co", [T, C], F32, isOutput=False)
    w_d = nc.declare_dram_parameter("wif", [C, IF], F32, isOutput=False)
    b_d = nc.declare_dram_parameter("bif", [1, IF], F32, isOutput=False)
    m0_d = nc.declare_dram_parameter("mem0", [N, W], F32, isOutput=False)
    ident_d = nc.declare_dram_parameter("ident", [128, 128], F32, isOutput=False)
    out_d = nc.declare_dram_parameter("out", [T, R, W], F32, isOutput=True)

    with tile.TileContext(nc) as tc:
        with (
            nc.allow_low_precision(reason="bf16 link + f32r broadcasts stay"
                                   " within the 2e-2 gate"),
            tc.tile_pool(name="const", bufs=1) as cp,
            tc.tile_pool(name="state", bufs=2) as sp,
            tc.tile_pool(name="work", bufs=2) as wp,
            tc.tile_pool(name="psBig", bufs=1, space="PSUM") as psB,
            tc.tile_pool(name="psMem", bufs=1, space="PSUM") as psM,
            tc.tile_pool(name="psS", bufs=2, space="PSUM") as psS,
        ):
            _build_body(nc, tc, cp, sp, wp, psB, psM, psS,
                        co_d, w_d, b_d, m0_d, ident_d, out_d)
    nc.compile()
    return nc


def _build_body(nc, tc, cp, sp, wp, psB, psM, psS,
                co_d, w_d, b_d, m0_d, ident_d, out_d):
    V, S, P, G_, DMA = nc.vector, nc.scalar, nc.tensor, nc.gpsimd, nc.sync

    # ---------------- constants ----------------
    ident = cp.tile([128, 128], F32)
    DMA.dma_start(ident[:], ident_d[:])
    ones = cp.tile([128, 128], F32)
    G_.memset(ones[:], 1.0)

    # persistent per-t tables
    iface = cp.tile([T, IF], F32)          # raw iface rows
    wvR = cp.tile([1, T, W], F32)          # write vectors, partition-0 rows
    keysc = cp.tile([W, 5, T], F32)        # scaled keys: r=0..3 read, 4 write
    neg_er = cp.tile([W, T], F32)
    gr = cp.tile([1, 6, T], F32)           # sigmoids: fg x4, ag, wg
    c1p = cp.tile([1, T], F32)
    cn1 = cp.tile([1, T], F32)
    c2 = cp.tile([1, T], F32)
    modes1 = cp.tile([1, R, T], F32)       # content-mode row per t
    mbs0 = cp.tile([128, R, T], F32)
    mbs2 = cp.tile([128, R, T], F32)
    nege0_pm = cp.tile([128, NT], F32)
    G_.memset(nege0_pm[:], 0.0)
    G_.memset(nege0_pm[0:1, 0:1], -1.0)
    out_sb = cp.tile([W, T, R], F32)

    # ---------------- prologue ----------------
    with tc.tile_pool(name="prolog", bufs=1) as pp:
        co_sb = pp.tile([T, C], F32)
        DMA.dma_start(co_sb[:], co_d[:])
        bif_sb = pp.tile([1, IF], F32)
        DMA.dma_start(bif_sb[:], b_d[:])
        w_sb = pp.tile([128, 8, IF], F32)
        for k in range(8):
            # split the 1.9MB load across two hwdge queues
            eng = DMA if k % 2 == 0 else nc.scalar
            eng.dma_start(w_sb[:, k, :], w_d[128 * k:128 * (k + 1), :])

        coT_p = psB.tile([128, 8, T], F32, tag="wb")
        for k in range(8):
            P.transpose(coT_p[:, k, :], co_sb[:, 128 * k:128 * (k + 1)],
                        ident[0:T, 0:T])
        coT = pp.tile([128, 8, T], F32)
        V.tensor_copy(coT[:], coT_p[:])

        if_p = psB.tile([T, IF], F32, tag="pb", bufs=2)
        for k in range(8):
            P.matmul(if_p[:], fr(coT[:, k, :]), fr(w_sb[:, k, :]),
                     start=(k == 0), stop=False)
        P.matmul(if_p[:], ones[0:1, 0:T], bif_sb[:],
                 start=False, stop=True)
        V.tensor_copy(iface[:], if_p[:])

        # keys [64, 5, T]: read r=0..3, write at 4
        keys_p = psB.tile([W, 5, T], F32, tag="pb", bufs=2)
        for r in range(R):
            P.transpose(keys_p[:, r, :], iface[:, O_RK + W * r:O_RK + W * (r + 1)],
                        ident[0:T, 0:T])
        P.transpose(keys_p[:, 4, :], iface[:, O_WK:O_WK + W], ident[0:T, 0:T])
        keys = pp.tile([W, 5, T], F32)
        V.tensor_copy(keys[:], keys_p[:])

        # write vectors as partition-0 rows via selector matmuls, two copies
        for h in range(2):
            wv_p = psB.tile([1, 8, W], F32, tag="pb", bufs=2, name=f"wvp{h}")
            for j in range(8):
                tt_ = 8 * h + j
                P.matmul(wv_p[0:1, j, :], ident[0:T, tt_:tt_ + 1],
                         iface[:, O_WV:O_WV + W])
            V.tensor_copy(wvR[0:1, 8 * h:8 * (h + 1), :].rearrange(
                "o t w -> o (t w)"),
                wv_p[:].rearrange("o t w -> o (t w)"))

        # erase sigmoid -> neg_er
        er_p = psS.tile([W, T], F32, tag="s")
        P.transpose(er_p[:], iface[:, O_ER:O_ER + W], ident[0:T, 0:T])
        ee = pp.tile([W, T], F32)
        S.activation(ee[:], er_p[:], AF.Exp, scale=-1.0)
        ew = pp.tile([W, T], F32)
        V.tensor_scalar(ew[:], ee[:], 1.0, None, op0=OP.add)
        er_r = pp.tile([W, T], F32)
        V.reciprocal(er_r[:], ew[:])
        V.tensor_scalar(neg_er[:], er_r[:], -1.0, None, op0=OP.mult)

        # strengths softplus: [1, 5, T] (rs x4, ws)
        sts_p = psS.tile([1, 5, T], F32, tag="s")
        for r in range(R):
            P.transpose(sts_p[0:1, r, :], iface[:, O_RS + r:O_RS + r + 1],
                        ident[0:T, 0:T])
        P.transpose(sts_p[0:1, 4, :], iface[:, O_WS:O_WS + 1], ident[0:T, 0:T])
        st_e = pp.tile([1, 5 * T], F32)
        S.activation(st_e[:], sts_p[:].rearrange("o f t -> o (f t)"), AF.Exp)
        st_w = pp.tile([1, 5 * T], F32)
        V.tensor_scalar(st_w[:], st_e[:], 1.0, None, op0=OP.add)
        st_sp = pp.tile([1, 5 * T], F32)
        S.activation(st_sp[:], st_w[:], AF.Ln)

        # key norms: rsqrt(sum keys^2) = exp(-0.5 ln)
        sqk = pp.tile([W, 5 * T], F32)
        S.activation(sqk[:], keys[:].rearrange("w f t -> w (f t)"), AF.Square)
        k2_p = psM.tile([1, 5 * T], F32, tag="wwb")
        P.matmul(k2_p[:], ones[0:W, 0:1], sqk[:])
        lk2 = pp.tile([1, 5 * T], F32)
        S.activation(lk2[:], k2_p[:], AF.Ln)
        kr = pp.tile([1, 5 * T], F32)
        S.activation(kr[:], lk2[:], AF.Exp, scale=-0.5)
        beta = pp.tile([1, 5 * T], F32)
        V.tensor_tensor(beta[:], st_sp[:], kr[:], op=OP.mult)
        kb_p = psM.tile([W, 5 * T], F32, tag="add")
        P.matmul(kb_p[:], ones[0:1, 0:W], beta[:])
        V.tensor_tensor(keysc[:].rearrange("w f t -> w (f t)"),
                        keys[:].rearrange("w f t -> w (f t)"), kb_p[:],
                        op=OP.mult)

        # gates: fg x4, ag, wg sigmoids
        gats_p = psS.tile([1, 6, T], F32, tag="s")
        for r in range(R):
            P.transpose(gats_p[0:1, r, :], iface[:, O_FG + r:O_FG + r + 1],
                        ident[0:T, 0:T])
        P.transpose(gats_p[0:1, 4, :], iface[:, O_AG:O_AG + 1], ident[0:T, 0:T])
        P.transpose(gats_p[0:1, 5, :], iface[:, O_WG:O_WG + 1], ident[0:T, 0:T])
        g_e = pp.tile([1, 6 * T], F32)
        S.activation(g_e[:], gats_p[:].rearrange("o g t -> o (g t)"), AF.Exp,
                     scale=-1.0)
        g_w = pp.tile([1, 6 * T], F32)
        V.tensor_scalar(g_w[:], g_e[:], 1.0, None, op0=OP.add)
        V.reciprocal(gr[:].rearrange("o g t -> o (g t)"), g_w[:])
        ag_t = gr[0:1, 4, :]
        wg_t = gr[0:1, 5, :]
        V.tensor_tensor(c1p[:], ag_t, wg_t, op=OP.mult)
        V.tensor_scalar(cn1[:], c1p[:], -1.0, None, op0=OP.mult)
        V.tensor_tensor(c2[:], wg_t, c1p[:], op=OP.subtract)

        # modes softmax -> rows per t
        me = pp.tile([T, 12], F32)
        S.activation(me[:], iface[:, O_MD:O_MD + 12], AF.Exp)
        me3 = me[:].rearrange("t (r m) -> t r m", m=3)
        msum = pp.tile([T, R], F32)
        V.tensor_tensor(msum[:], me3[:, :, 0], me3[:, :, 1], op=OP.add)
        V.tensor_tensor(msum[:], msum[:], me3[:, :, 2], op=OP.add)
        mrcp = pp.tile([T, R], F32)
        V.reciprocal(mrcp[:], msum[:])
        mn = pp.tile([T, 12], F32)
        mn3 = mn[:].rearrange("t (m r) -> t m r", r=R)
        me3b = me[:].rearrange("t (r m) -> t m r", m=3)
        for m in range(3):
            V.tensor_tensor(mn3[:, m, :], me3b[:, m, :], mrcp[:], op=OP.mult)
        # three m-blocks at base partition 0: modes0/1/2 [4, T]
        mblk_p = psS.tile([R, 3, T], F32, tag="s")
        for m in range(3):
            P.transpose(mblk_p[:, m, :], mn[:, 4 * m:4 * (m + 1)],
                        ident[0:T, 0:T])
        mblk = pp.tile([R, 3, T], F32)
        V.tensor_copy(mblk[:], mblk_p[:])
        m1sel_p = psS.tile([1, R, T], F32, tag="s")
        for r in range(R):
            P.matmul(m1sel_p[0:1, r, :], ident[0:R, r:r + 1], mblk[:, 1, :])
        V.tensor_copy(modes1[:].rearrange("o r t -> o (r t)"),
                      m1sel_p[:].rearrange("o r t -> o (r t)"))
        # flatten rows r of m-block 0/2 onto partition 0 via selector matmuls
        mrows_p = psS.tile([1, 2, R, T], F32, tag="s")
        for r in range(R):
            P.matmul(mrows_p[0:1, 0, r, :], ident[0:R, r:r + 1], mblk[:, 0, :])
            P.matmul(mrows_p[0:1, 1, r, :], ident[0:R, r:r + 1], mblk[:, 2, :])
        mrows = pp.tile([1, 2, R, T], F32)
        V.tensor_copy(mrows[:].rearrange("o a r t -> o (a r t)"),
                      mrows_p[:].rearrange("o a r t -> o (a r t)"))
        mb0_p = psB.tile([128, R * T], F32, tag="wb")
        P.matmul(mb0_p[:], ones[0:1, :], mrows[0:1, 0, :, :])
        V.tensor_copy(mbs0[:].rearrange("p r t -> p (r t)"), mb0_p[:])
        mb2_p = psB.tile([128, R * T], F32, tag="pb", bufs=2)
        P.matmul(mb2_p[:], ones[0:1, :], mrows[0:1, 1, :, :])
        V.tensor_copy(mbs2[:].rearrange("p r t -> p (r t)"), mb2_p[:])

    # ---------------- initial state ----------------
    mem_nrm = sp.tile([128, NT, W], F32, tag="mem_nrm")
    for c in range(NT):
        DMA.dma_start(mem_nrm[:, c, :],
                      m0_d[128 * c:128 * (c + 1), :])
    memT_p = psB.tile([W, N], F32, tag="wb")
    for c in range(NT):
        P.transpose(memT_p[:, 128 * c:128 * (c + 1)],
                    mem_nrm[:, c, :], ident[:])
    memT = sp.tile([W, N], F32, tag="memT")
    V.tensor_copy(memT[:], memT_p[:])

    # initial norm: PM-layout sqn -> msq -> Ln/Exp
    sqn0 = wp.tile([128, NT, W], F32, tag="sqn")
    G_.tensor_tensor(sqn0[:], mem_nrm[:], mem_nrm[:], op=OP.mult)
    msq0 = wp.tile([128, NT], F32, tag="msq")
    V.tensor_reduce(msq0[:], sqn0[:], axis=mybir.AxisListType.X, op=OP.add)
    lms0 = wp.tile([128, NT], F32, tag="lms")
    S.activation(lms0[:], msq0[:], AF.Ln)
    mnorm_i = sp.tile([128, NT], F32, tag="mnorm")
    S.activation(mnorm_i[:], lms0[:], AF.Exp, scale=-0.5)

    L = sp.tile([128, NT, N], BF16, tag="L")
    G_.memset(L[:], 0.0)
    LT0 = sp.tile([128, NT, N], BF16, tag="LT")
    G_.memset(LT0[:], 0.0)
    dL0 = sp.tile([128, NT], F32, tag="dL")
    G_.memset(dL0[:], 0.0)

    st = dict(memT=memT, mem_nrm=mem_nrm, mnorm=mnorm_i, L=L, LT=LT0,
              dL=dL0, u_pm=None, prec_pm=None, prec_fl=None,
              pbs=None, rwT=None)

    for t in range(T):
        st = _step(nc, t, st, cp, sp, wp, psB, psM, psS,
                   ident, ones, iface, wvR, keysc, neg_er, gr, c1p, cn1, c2,
                   modes1, mbs0, mbs2, nege0_pm, out_sb)

    DMA.dma_start(out_d[:].rearrange("t r w -> w t r"), out_sb[:])


def _step(nc, t, st, cp, sp, wp, psB, psM, psS,
          ident, ones, iface, wvR, keysc, neg_er, gr, c1p, cn1, c2,
          modes1, mbs0, mbs2, nege0_pm, out_sb):
    V, S, P, G_, DMA = nc.vector, nc.scalar, nc.tensor, nc.gpsimd, nc.sync
    memT, mem_nrm, mnorm = st["memT"], st["mem_nrm"], st["mnorm"]
    L, LT, dL, u_pm = st["L"], st["LT"], st["dL"], st["u_pm"]
    prec_pm, prec_fl, pbs, rwT = (st["prec_pm"], st["prec_fl"], st["pbs"],
                                  st["rwT"])
    last = (t == T - 1)

    if t == 0:
        na_pm, nsa = nege0_pm, None
    else:
        na_pm, nsa = st["na_pm"], st["nsa"]

    # ---- pbs broadcast (prev-step prec; runs at step start) ----
    if t > 0:
        pb_p = psB.tile([128, N], F32, tag="pb", bufs=2)
        P.matmul(pb_p[:], fr(ones[0:1, :]), fr(prec_fl[:]))
        pbs = wp.tile([128, N], BF16, tag="pbs")
        S.activation(pbs[:], pb_p[:], AF.Copy)

    # ---- write content softmax (PM) ----
    wdots_p = psS.tile([128, NT], F32, tag="s")
    for b in range(NT):
        P.matmul(wdots_p[:, b:b + 1], memT[:, 128 * b:128 * (b + 1)],
                 keysc[:, 4, t:t + 1])
    wlog = wp.tile([128, NT], F32, tag="wlog")
    V.tensor_tensor(wlog[:], wdots_p[:], mnorm[:], op=OP.mult)
    wexp_pm = wp.tile([128, NT], F32, tag="wexp")
    S.activation(wexp_pm[:], wlog[:], AF.Exp)
    wps_p = psS.tile([1, NT], F32, tag="s")
    P.matmul(wps_p[:], ones[:, 0:1], wexp_pm[:])
    wsum = wp.tile([1, 1], F32, tag="wsum")
    V.tensor_reduce(wsum[:], wps_p[:], axis=mybir.AxisListType.X, op=OP.add)
    wrs = wp.tile([1, 1], F32, tag="wrs")
    V.reciprocal(wrs[:], wsum[:])
    cw = wp.tile([1, 1], F32, tag="cw")
    V.tensor_tensor(cw[:], wrs[:], c2[0:1, t:t + 1], op=OP.mult)

    # ---- ww assembly (PM) ----
    cn1b_p = psS.tile([128, 1], F32, tag="s")
    P.matmul(cn1b_p[:], ones[0:1, :], cn1[0:1, t:t + 1])
    cwb_p = psS.tile([128, 1], F32, tag="s")
    P.matmul(cwb_p[:], ones[0:1, :], cw[:])
    wwx = wp.tile([128, NT], F32, tag="wwx")
    V.tensor_scalar(wwx[:], na_pm[:], cn1b_p[:, 0:1], None, op0=OP.mult)
    ww_pm = wp.tile([128, NT], F32, tag="wwpm")
    V.scalar_tensor_tensor(ww_pm[:], wexp_pm[:], cwb_p[:, 0:1], wwx[:],
                           op0=OP.mult, op1=OP.add)
    if t > 0:
        omw_pm = wp.tile([128, NT], F32, tag="omw")
        V.tensor_scalar(omw_pm[:], ww_pm[:], -1.0, 1.0, op0=OP.mult,
                        op1=OP.add)
    ww_tp = psS.tile([1, N], F32, tag="s")
    for c in range(NT):
        P.transpose(ww_tp[0:1, 128 * c:128 * (c + 1)], ww_pm[:, c:c + 1],
                    ident[:])
    ww_fl = wp.tile([1, N], F32, tag="wwfl")
    S.copy(ww_fl[:], ww_tp[:])

    # ---- wbs broadcast (bf16, for the link ts ops) ----
    if t > 0:
        wb_p = psB.tile([128, N], F32, tag="wb")
        P.matmul(wb_p[:], fr(ones[0:1, :]), fr(ww_fl[:]))
        wbs = wp.tile([128, N], BF16, tag="wbs")
        S.activation(wbs[:], wb_p[:], AF.Copy)

    # ---- memory head ----
    wwb_p = psM.tile([W, N], F32, tag="wwb")
    P.matmul(wwb_p[:], fr(ones[0:1, 0:W]), fr(ww_fl[:]))
    add_p = psM.tile([W, N], F32, tag="add")
    P.matmul(add_p[:], fr(wvR[0:1, t, :]), fr(ww_fl[:]))
    keep = wp.tile([W, N], F32, tag="keep")
    S.activation(keep[:], wwb_p[:], AF.Copy, scale=neg_er[:, t:t + 1],
                 bias=1.0)
    m1 = wp.tile([W, N], F32, tag="m1")
    G_.tensor_tensor(m1[:], memT[:], keep[:], op=OP.mult)

    # ---- usage update ----
    if last:
        u_pm_n = u_pm
    else:
        u_pm_n = sp.tile([128, NT], F32, tag="u_pm")
        if t == 0:
            V.tensor_copy(u_pm_n[:], ww_pm[:])
        else:
            fgb_p = psS.tile([128, R], F32, tag="s")
            P.matmul(fgb_p[:], ones[0:1, :], gr[0:1, 0:R, t])
            yyT = wp.tile([128, NT, R], F32, tag="yyT")
            V.scalar_tensor_tensor(
                yyT[:], fgb_p[:, None, :].broadcast_to([128, NT, R]), -1.0,
                rwT[:].rearrange("p (c r) -> p c r", r=R),
                op0=OP.mult, op1=OP.mult)
            om = wp.tile([128, NT, R], F32, tag="om")
            V.tensor_scalar(om[:], yyT[:], 1.0, None, op0=OP.add)
            p1u = wp.tile([128, NT], F32, tag="p1u")
            G_.tensor_tensor(p1u[:], om[:, :, 0], om[:, :, 1], op=OP.mult)
            p2u = wp.tile([128, NT], F32, tag="p2u")
            G_.tensor_tensor(p2u[:], om[:, :, 2], om[:, :, 3], op=OP.mult)
            psi = wp.tile([128, NT], F32, tag="psi")
            G_.tensor_tensor(psi[:], p1u[:], p2u[:], op=OP.mult)
            omu = wp.tile([128, NT], F32, tag="omu")
            V.tensor_scalar(omu[:], u_pm[:], -1.0, 1.0, op0=OP.mult,
                            op1=OP.add)
            tn = wp.tile([128, NT], F32, tag="tn")
            V.scalar_tensor_tensor(tn[:], ww_pm[:], 1.0, omu[:],
                                   op0=OP.subtract, op1=OP.mult)
            V.scalar_tensor_tensor(u_pm_n[:], tn[:], 1.0, psi[:],
                                   op0=OP.add, op1=OP.mult)

    # ---- allocation compare inputs (flat u + broadcast; exact fp32) ----
    if not last:
        u_tp = psS.tile([1, N], F32, tag="s")
        for c in range(NT):
            P.transpose(u_tp[0:1, 128 * c:128 * (c + 1)], u_pm_n[:, c:c + 1],
                        ident[:])
        u_fl_n = wp.tile([1, N], F32, tag="ufl")
        V.tensor_copy(u_fl_n[:], u_tp[:])
        ub_p = psM.tile([128, N], F32, tag="wwb")
        P.matmul(ub_p[:], ones[0:1, :], u_fl_n[:])
        ubs = wp.tile([128, N], F32, tag="ubs")
        S.copy(ubs[:], ub_p[:])
        ucl = wp.tile([128, NT], F32, tag="ucl")
        V.tensor_scalar(ucl[:], u_pm_n[:], 1e-38, None, op0=OP.max)
        lnu = wp.tile([128, NT], F32, tag="lnu")
        S.activation(lnu[:], ucl[:], AF.Ln)

    # ---- prec update ----
    if not last:
        prec_pm_n = sp.tile([128, NT], F32, tag="prec_pm")
        if t == 0:
            V.tensor_copy(prec_pm_n[:], ww_pm[:])
        else:
            swa = wp.tile([1, 1], F32, tag="swa")
            G_.tensor_tensor(swa[:], nsa[:], cn1[0:1, t:t + 1], op=OP.mult)
            sw = wp.tile([1, 1], F32, tag="sw")
            G_.tensor_tensor(sw[:], swa[:], c2[0:1, t:t + 1], op=OP.add)
            omsw = wp.tile([1, 1], F32, tag="omsw")
            V.tensor_scalar(omsw[:], sw[:], -1.0, 1.0, op0=OP.mult,
                            op1=OP.add)
            omsw_p = psS.tile([128, 1], F32, tag="s")
            P.matmul(omsw_p[:], ones[0:1, :], omsw[:])
            V.scalar_tensor_tensor(prec_pm_n[:], prec_pm[:], omsw_p[:, 0:1],
                                   ww_pm[:], op0=OP.mult, op1=OP.add)
        p_tp = psS.tile([1, N], F32, tag="s")
        for c in range(NT):
            P.transpose(p_tp[0:1, 128 * c:128 * (c + 1)], prec_pm_n[:, c:c + 1],
                        ident[:])
        prec_fl_n = sp.tile([1, N], F32, tag="prec_fl")
        S.copy(prec_fl_n[:], p_tp[:])
    else:
        prec_pm_n, prec_fl_n = prec_pm, prec_fl

    # ---- mode-scaled read weights + link diagonal tracker ----
    if t > 0:
        rwTm0 = wp.tile([128, NT * R], BF16, tag="rwTm0")
        V.tensor_tensor(rwTm0[:].rearrange("p (c r) -> p c r", r=R),
                        rwT[:].rearrange("p (c r) -> p c r", r=R),
                        mbs0[:, None, :, t].broadcast_to([128, NT, R]),
                        op=OP.mult)
        rwTm2 = wp.tile([128, NT * R], BF16, tag="rwTm2")
        V.tensor_tensor(rwTm2[:].rearrange("p (c r) -> p c r", r=R),
                        rwT[:].rearrange("p (c r) -> p c r", r=R),
                        mbs2[:, None, :, t].broadcast_to([128, NT, R]),
                        op=OP.mult)
        wp_pm = wp.tile([128, NT], F32, tag="wppm")
        G_.tensor_tensor(wp_pm[:], ww_pm[:], prec_pm[:], op=OP.mult)
        dmul = wp.tile([128, NT], F32, tag="dmul")
        V.tensor_scalar(dmul[:], ww_pm[:], -2.0, 1.0, op0=OP.mult, op1=OP.add)
        dL_n = sp.tile([128, NT], F32, tag="dL")
        V.scalar_tensor_tensor(dL_n[:], dL[:], 1.0, dmul[:],
                               op0=OP.mult, op1=OP.mult)
        G_.tensor_tensor(dL_n[:], dL_n[:], wp_pm[:], op=OP.add)
    else:
        dL_n = dL

    # ---- link loop with interleaved memT_n / Gt compares ----
    comb_eng = [(G_, G_), (G_, V), (G_, V), (G_, V)]
    if t == 0:
        L_n, LT_n = L, LT
        memT_n = sp.tile([W, N], F32, tag="memT")
        V.tensor_tensor(memT_n[:], m1[:], add_p[:], op=OP.add)
        if not last:
            Gt_n = wp.tile([128, NT, N], F32, tag="G", bufs=1)
            for c in range(NT):
                V.tensor_scalar(Gt_n[:, c, :], ubs[:], u_pm_n[:, c:c + 1],
                                None, op0=OP.is_gt)
    else:
        L_n = sp.tile([128, NT, N], BF16, tag="L")
        LT_n = sp.tile([128, NT, N], BF16, tag="LT")
        memT_n = sp.tile([W, N], F32, tag="memT")
        if not last:
            Gt_n = wp.tile([128, NT, N], F32, tag="G", bufs=1)
        for c in range(NT):
            w1 = wp.tile([128, N], BF16, tag=f"w1_{c % 2}")
            V.tensor_scalar(w1[:], wbs[:], omw_pm[:, c:c + 1], None,
                            op0=OP.subtract)
            p1 = wp.tile([128, N], BF16, tag=f"p1_{c % 2}")
            V.tensor_scalar(p1[:], pbs[:], ww_pm[:, c:c + 1], None,
                            op0=OP.mult)
            p1T = wp.tile([128, N], BF16, tag=f"p1T_{c % 2}")
            V.tensor_scalar(p1T[:], wbs[:], prec_pm[:, c:c + 1], None,
                            op0=OP.mult)
            t1 = wp.tile([128, N], BF16, tag=f"t1_{c % 2}")
            G_.tensor_tensor(t1[:], w1[:], L[:, c, :], op=OP.mult)
            t1T = wp.tile([128, N], BF16, tag=f"t1T_{c % 2}")
            G_.tensor_tensor(t1T[:], w1[:], LT[:, c, :], op=OP.mult)
            eL, eLT = comb_eng[c]
            eL.tensor_tensor(L_n[:, c, :], p1[:], t1[:], op=OP.subtract)
            eLT.tensor_tensor(LT_n[:, c, :], p1T[:], t1T[:], op=OP.subtract)
            if c == 0:
                V.tensor_tensor(memT_n[:], m1[:], add_p[:], op=OP.add)
            elif not last:
                cc = c - 1
                V.tensor_scalar(Gt_n[:, cc, :], ubs[:],
                                u_pm_n[:, cc:cc + 1], None, op0=OP.is_gt)
        if not last:
            V.tensor_scalar(Gt_n[:, 3, :], ubs[:], u_pm_n[:, 3:4],
                            None, op0=OP.is_gt)

    # ---- memory norm chain ----
    mem_nrm_p = psS.tile([128, NT, W], F32, tag="mn", bufs=1)
    for c in range(NT):
        P.transpose(mem_nrm_p[:, c, :], memT_n[:, 128 * c:128 * (c + 1)],
                    ident[0:W, 0:W])
    sqn = wp.tile([128, NT, W], F32, tag="sqn")
    S.activation(sqn[:], mem_nrm_p[:], AF.Square)
    msq = wp.tile([128, NT], F32, tag="msq")
    V.tensor_reduce(msq[:], sqn[:], axis=mybir.AxisListType.X, op=OP.add)
    mem_nrm_n = sp.tile([128, NT, W], F32, tag="mem_nrm")
    S.copy(mem_nrm_n[:], mem_nrm_p[:])
    lms = wp.tile([128, NT], F32, tag="lms")
    S.activation(lms[:], msq[:], AF.Ln)
    mnorm_n = sp.tile([128, NT], F32, tag="mnorm")
    S.activation(mnorm_n[:], lms[:], AF.Exp, scale=-0.5)

    # ---- allocation log-sum ----
    if last:
        na_pm_n, nsa_n = None, None
    else:
        sT_p = psS.tile([128, NT], F32, tag="s")
        for b in range(NT):
            for c in range(NT):
                P.matmul(sT_p[:, b:b + 1], Gt_n[:, c, 128 * b:128 * (b + 1)],
                         lnu[:, c:c + 1], start=(c == 0),
                         stop=(c == NT - 1))
        es_pm = wp.tile([128, NT], F32, tag="espm")
        S.activation(es_pm[:], sT_p[:], AF.Exp)
        na_pm_n = wp.tile([128, NT], F32, tag="napm")
        if t < T - 2:
            nap = wp.tile([128, 1], F32, tag="nap")
            V.scalar_tensor_tensor(na_pm_n[:], u_pm_n[:], 1.0, es_pm[:],
                                   op0=OP.subtract, op1=OP.mult,
                                   accum_out=nap[:])
        else:
            V.scalar_tensor_tensor(na_pm_n[:], u_pm_n[:], 1.0, es_pm[:],
                                   op0=OP.subtract, op1=OP.mult)
        if t < T - 2:
            nsa_p = psS.tile([1, 1], F32, tag="s")
            P.matmul(nsa_p[:], nap[:], ones[:, 0:1])
            nsa_n = wp.tile([1, 1], F32, tag="nsa")
            V.tensor_copy(nsa_n[:], nsa_p[:])
        else:
            nsa_n = None

    # ---- read content (PM) ----
    rdots_p = psS.tile([128, NT * R], F32, tag="s")
    for b in range(NT):
        P.matmul(rdots_p[:, R * b:R * (b + 1)],
                 memT_n[:, 128 * b:128 * (b + 1)], keysc[:, 0:4, t])
    rlog = wp.tile([128, NT, R], F32, tag="rlog")
    V.tensor_tensor(rlog[:],
                    rdots_p[:].rearrange("p (c r) -> p c r", r=R),
                    mnorm_n[:, :, None].broadcast_to([128, NT, R]),
                    op=OP.mult)
    rexp_pm = wp.tile([128, NT * R], F32, tag="rexp")
    S.activation(rexp_pm[:], rlog[:].rearrange("p c r -> p (c r)"), AF.Exp)
    rps_p = psS.tile([1, NT * R], F32, tag="s")
    P.matmul(rps_p[:], ones[:, 0:1], rexp_pm[:])
    rsum = wp.tile([1, R], F32, tag="rsum")
    V.tensor_reduce(rsum[:], rps_p[:].rearrange("o (c r) -> o r c", r=R),
                    axis=mybir.AxisListType.X, op=OP.add)
    rsr = wp.tile([1, R], F32, tag="rsr")
    V.reciprocal(rsr[:], rsum[:])
    s1c = wp.tile([1, R], F32, tag="s1c")
    V.tensor_tensor(s1c[:], rsr[:], modes1[0:1, :, t], op=OP.mult)
    s1cb_p = psS.tile([128, R], F32, tag="s")
    P.matmul(s1cb_p[:], ones[0:1, :], s1c[:])

    cnt = wp.tile([128, NT, R], F32, tag="cnt")
    V.tensor_tensor(cnt[:], rexp_pm[:].rearrange("p (c r) -> p c r", r=R),
                    s1cb_p[:, None, :].broadcast_to([128, NT, R]), op=OP.mult)
    rwT_n = sp.tile([128, NT * R], F32, tag="rwT")
    if t > 0:
        rwT_p = psS.tile([128, NT * R], F32, tag="s")
        for b in range(NT):
            blk = slice(128 * b, 128 * (b + 1))
            for c in range(NT):
                P.matmul(rwT_p[:, R * b:R * (b + 1)], L_n[:, c, blk],
                         rwTm0[:, R * c:R * (c + 1)],
                         start=(c == 0), stop=False)
            for c in range(NT):
                P.matmul(rwT_p[:, R * b:R * (b + 1)], LT_n[:, c, blk],
                         rwTm2[:, R * c:R * (c + 1)],
                         start=False, stop=(c == NT - 1))
        s02 = wp.tile([128, NT, R], F32, tag="s02")
        G_.tensor_tensor(s02[:], rwTm0[:].rearrange("p (c r) -> p c r", r=R),
                        rwTm2[:].rearrange("p (c r) -> p c r", r=R),
                        op=OP.add)
        corr = wp.tile([128, NT, R], F32, tag="corr")
        V.tensor_tensor(corr[:], s02[:],
                        dL_n[:, :, None].broadcast_to([128, NT, R]),
                        op=OP.mult)
        cnt2 = wp.tile([128, NT, R], F32, tag="cnt2")
        G_.tensor_tensor(cnt2[:], cnt[:], corr[:], op=OP.subtract)
        V.tensor_tensor(rwT_n[:], cnt2[:].rearrange("p c r -> p (c r)"),
                        rwT_p[:], op=OP.add)
    else:
        V.tensor_copy(rwT_n[:], cnt[:].rearrange("p c r -> p (c r)"))

    rwd_p = psS.tile([W, R], F32, tag="s")
    for c in range(NT):
        P.matmul(rwd_p[:], mem_nrm_n[:, c, :],
                 rwT_n[:, R * c:R * (c + 1)],
                 start=(c == 0), stop=(c == NT - 1))
    V.tensor_copy(out_sb[:, t, :], rwd_p[:])

    return dict(memT=memT_n, mem_nrm=mem_nrm_n, mnorm=mnorm_n, L=L_n,
                LT=LT_n, dL=dL_n, u_pm=u_pm_n, na_pm=na_pm_n,
                nsa=nsa_n, prec_pm=prec_pm_n, prec_fl=prec_fl_n, pbs=pbs,
                rwT=rwT_n)


# ---------------------------------------------------------------------------
_NC_CACHE = {}


def _get_nc():
    if "nc" not in _NC_CACHE:
        _NC_CACHE["nc"] = build_nc()
    return _NC_CACHE["nc"]


def _consts():
    ident = np.eye(128, dtype=np.float32)
    return (ident,)


def make_in_maps(controller_output, W_if, b_if, memory0):
    (ident,) = _consts()
    maps = []
    for b in range(B):
        maps.append({
            "co": np.ascontiguousarray(controller_output[b]),
            "wif": np.ascontiguousarray(W_if),
            "bif": np.ascontiguousarray(b_if.reshape(1, IF)),
            "mem0": np.ascontiguousarray(memory0[b]),
            "ident": ident,
        })
    return maps


def kernel(controller_output, W_if, b_if, memory0):
    from concourse.bass_utils import run_bass_kernel_spmd
    controller_output = np.asarray(controller_output, dtype=np.float32)
    W_if = np.asarray(W_if, dtype=np.float32)
    b_if = np.asarray(b_if, dtype=np.float32)
    memory0 = np.asarray(memory0, dtype=np.float32)
    nc = _get_nc()
    maps = make_in_maps(controller_output, W_if, b_if, memory0)
    res = run_bass_kernel_spmd(nc, maps, core_ids=list(range(B)))
    return np.stack([res.results[b]["out"] for b in range(B)], axis=0)


if __name__ == "__main__":
    mode = sys.argv[1] if len(sys.argv) > 1 else "sim"
    sys.path.insert(0, "/root/problem")
    import jax
    with jax.default_device(jax.devices("cpu")[0]):
        import reference
        inputs = {k: np.asarray(v) for k, v in reference.setup_inputs().items()}
        expected = np.asarray(reference.reference(**inputs))

    if mode == "sim":
        from concourse.bass_interp import CoreSim
        nc = build_nc()
        maps = make_in_maps(inputs["controller_output"], inputs["W_if"],
                            inputs["b_if"], inputs["memory0"])
        sim = CoreSim(nc)
        for k, v in maps[0].items():
            sim.tensor(k)[:] = v
        sim.simulate()
        got = sim.tensor("out").copy()
        exp = expected[0]
        err = np.abs(got - exp)
        rel = np.linalg.norm(got - exp) / (np.linalg.norm(exp) + 1e-12)
        print("sim modeled time (ns):", sim.time)
        print("max abs err:", err.max(), " rel err:", rel)
    else:
        got = kernel(**inputs)
        rel = np.linalg.norm(got - expected) / (np.linalg.norm(expected) + 1e-12)
        print("max abs err:", np.abs(got - expected).max(), " rel err:", rel)


# revision 9
# speedup vs baseline: 1.6806x; 1.1496x over previous
"""DNC MemoryAccess kernel for Trainium2 (Bass/Tile), data-parallel over batch.

Shapes (hardcoded): B=8, T=16, C=1024, IFACE=471, N=512, WORD=64, R=4, NW=1.
Each of the 8 cores processes one batch element; all recurrent state stays
SBUF-resident across the T=16 sequential steps.

v2 redesign vs the previous kernel:
- temporal link L and its transpose LT are kept in bf16; their elementwise
  recurrence uses shared w1 = (w_j - (1-w_i)) via fast-mode tensor_scalar
  (0.25x DVE cycles in bf16) plus tensor_tensor combines split across
  DVE/Pool,
- the link diagonal is never fixed up in-place: the scalar diagonal
  recurrence d = (1-2w)d + w*p is tracked separately ([128,NT]) and its
  contribution subtracted from the fwd/bwd matmul results,
- allocation ln(usage) and the memory-norm rsqrt use the Activation table
  Ln/Exp (one act-func set covers Exp/Ln/Copy/Square/Sign),
- broadcast matmuls and the iface GEMM run as float32r (1 cycle/row at
  >=256 free elems); the usage-compare broadcast stays exact fp32 so the
  allocation sort ties match the fp32 reference,
- u_fl/prec_fl PM->flat flattens are SBUF->SBUF DMAs (4 column descriptors
  each) issued on the SP queue: zero compute-engine cost,
- PSUM->SBUF copies land on the Activation engine, elementwise memory-update
  work on Pool, everything latency-critical stays on DVE.
"""
import sys

sys.path.insert(0, "/opt/trn_rl_repo")

import numpy as np

import concourse.bacc as bacc
import concourse.bass as bass
import concourse.mybir as mybir
import concourse.tile as tile

F32 = mybir.dt.float32
F32R = mybir.dt.float32r
BF16 = mybir.dt.bfloat16
I32 = mybir.dt.int32
AF = mybir.ActivationFunctionType
OP = mybir.AluOpType

B, T, C, IF = 8, 16, 1024, 471
N, W, R = 512, 64, 4
NT = N // 128

O_RK, O_RS, O_WK, O_WS = 0, 256, 260, 324
O_ER, O_WV, O_FG, O_AG, O_WG, O_MD = 325, 389, 453, 457, 458, 459


def fr(ap):
    return ap


# Prefer the activation-function set that contains Exp AND Ln (plus
# Copy/Square/Sign), so the per-step Exp/Ln mix resolves to one table and the
# compiler hoists a single LoadActFuncSet out of the step loop instead of
# thrashing 1283ns loads between exp-only and ln-only sets.
_ORIG_GET_ACT_TABLES = None


def _patch_act_tables():
    global _ORIG_GET_ACT_TABLES
    if _ORIG_GET_ACT_TABLES is not None:
        return
    import concourse.hw_specs as hw_specs
    _ORIG_GET_ACT_TABLES = hw_specs.get_activation_tables

    def pinned(arch):
        tabs = dict(_ORIG_GET_ACT_TABLES(arch))
        pref = "natural_log_exp_and_others"
        if pref not in tabs:
            return tabs
        exp_ln = {mybir.ActivationFunctionType.Exp,
                  mybir.ActivationFunctionType.Ln}
        out = {}
        for k, v in tabs.items():
            out[k] = set(v) if k == pref else set(v) - exp_ln
        return out

    bacc.get_activation_tables = pinned


def build_nc():
    _patch_act_tables()
    nc = bacc.Bacc("TRN2", target_bir_lowering=False, debug=False, num_devices=8)

    co_d = nc.declare_dram_parameter("co", [T, C], F32, isOutput=False)
    w_d = nc.declare_dram_parameter("wif", [C, IF], F32, isOutput=False)
    b_d = nc.declare_dram_parameter("bif", [1, IF], F32, isOutput=False)
    m0_d = nc.declare_dram_parameter("mem0", [N, W], F32, isOutput=False)
    ident_d = nc.declare_dram_parameter("ident", [128, 128], F32, isOutput=False)
    out_d = nc.declare_dram_parameter("out", [T, R, W], F32, isOutput=True)

    with tile.TileContext(nc) as tc:
        with (
            nc.allow_low_precision(reason="bf16 link + f32r broadcasts stay"
                                   " within the 2e-2 gate"),
            tc.tile_pool(name="const", bufs=1) as cp,
            tc.tile_pool(name="state", bufs=2) as sp,
            tc.tile_pool(name="work", bufs=2) as wp,
            tc.tile_pool(name="psBig", bufs=1, space="PSUM") as psB,
            tc.tile_pool(name="psMem", bufs=1, space="PSUM") as psM,
            tc.tile_pool(name="psS", bufs=2, space="PSUM") as psS,
        ):
            _build_body(nc, tc, cp, sp, wp, psB, psM, psS,
                        co_d, w_d, b_d, m0_d, ident_d, out_d)
    nc.compile()
    return nc


def _build_body(nc, tc, cp, sp, wp, psB, psM, psS,
                co_d, w_d, b_d, m0_d, ident_d, out_d):
    V, S, P, G_, DMA = nc.vector, nc.scalar, nc.tensor, nc.gpsimd, nc.sync

    # ---------------- constants ----------------
    ident = cp.tile([128, 128], F32)
    DMA.dma_start(ident[:], ident_d[:])
    ones = cp.tile([128, 128], F32)
    G_.memset(ones[:], 1.0)
    ones_b = cp.tile([1, 128], BF16)
    G_.memset(ones_b[:], 1.0)

    # persistent per-t tables
    iface = cp.tile([T, IF], F32)          # raw iface rows
    wvR = cp.tile([1, T, W], BF16)         # write vectors, partition-0 rows
    keysc = cp.tile([W, 5, T], F32)        # scaled keys: r=0..3 read, 4 write
    neg_er = cp.tile([W, T], F32)
    gr = cp.tile([1, 6, T], F32)           # sigmoids: fg x4, ag, wg
    c1p = cp.tile([1, T], F32)
    cn1 = cp.tile([1, T], F32)
    c2 = cp.tile([1, T], F32)
    modes1 = cp.tile([1, R, T], F32)       # content-mode row per t
    mbs0 = cp.tile([128, R, T], F32)
    mbs2 = cp.tile([128, R, T], F32)
    nege0_pm = cp.tile([128, NT], F32)
    G_.memset(nege0_pm[:], 0.0)
    G_.memset(nege0_pm[0:1, 0:1], -1.0)
    out_sb = cp.tile([W, T, R], F32)

    # ---------------- prologue ----------------
    with tc.tile_pool(name="prolog", bufs=1) as pp:
        co_sb = pp.tile([T, C], F32)
        DMA.dma_start(co_sb[:], co_d[:])
        bif_sb = pp.tile([1, IF], F32)
        DMA.dma_start(bif_sb[:], b_d[:])
        w_sb = pp.tile([128, 8, IF], F32)
        for k in range(8):
            # split the 1.9MB load across two hwdge queues
            eng = DMA if k % 2 == 0 else nc.scalar
            eng.dma_start(w_sb[:, k, :], w_d[128 * k:128 * (k + 1), :])

        coT_p = psB.tile([128, 8, T], F32, tag="wb")
        for k in range(8):
            P.transpose(coT_p[:, k, :], co_sb[:, 128 * k:128 * (k + 1)],
                        ident[0:T, 0:T])
        coT = pp.tile([128, 8, T], F32)
        V.tensor_copy(coT[:], coT_p[:])

        if_p = psB.tile([T, IF], F32, tag="pb", bufs=2)
        for k in range(8):
            P.matmul(if_p[:], coT[:, k, :], w_sb[:, k, :],
                     start=(k == 0), stop=False)
        P.matmul(if_p[:], ones[0:1, 0:T], bif_sb[:],
                 start=False, stop=True)
        V.tensor_copy(iface[:], if_p[:])

        # keys [64, 5, T]: read r=0..3, write at 4
        keys_p = psB.tile([W, 5, T], F32, tag="pb", bufs=2)
        for r in range(R):
            P.transpose(keys_p[:, r, :], iface[:, O_RK + W * r:O_RK + W * (r + 1)],
                        ident[0:T, 0:T])
        P.transpose(keys_p[:, 4, :], iface[:, O_WK:O_WK + W], ident[0:T, 0:T])
        keys = pp.tile([W, 5, T], F32)
        V.tensor_copy(keys[:], keys_p[:])

        # write vectors as partition-0 rows via selector matmuls, two copies
        for h in range(2):
            wv_p = psB.tile([1, 8, W], F32, tag="pb", bufs=2, name=f"wvp{h}")
            for j in range(8):
                tt_ = 8 * h + j
                P.matmul(wv_p[0:1, j, :], ident[0:T, tt_:tt_ + 1],
                         iface[:, O_WV:O_WV + W])
            V.tensor_copy(wvR[0:1, 8 * h:8 * (h + 1), :].rearrange(
                "o t w -> o (t w)"),
                wv_p[:].rearrange("o t w -> o (t w)"))

        # erase sigmoid -> neg_er
        er_p = psS.tile([W, T], F32, tag="s")
        P.transpose(er_p[:], iface[:, O_ER:O_ER + W], ident[0:T, 0:T])
        ee = pp.tile([W, T], F32)
        S.activation(ee[:], er_p[:], AF.Exp, scale=-1.0)
        ew = pp.tile([W, T], F32)
        V.tensor_scalar(ew[:], ee[:], 1.0, None, op0=OP.add)
        er_r = pp.tile([W, T], F32)
        V.reciprocal(er_r[:], ew[:])
        V.tensor_scalar(neg_er[:], er_r[:], -1.0, None, op0=OP.mult)

        # strengths softplus: [1, 5, T] (rs x4, ws)
        sts_p = psS.tile([1, 5, T], F32, tag="s")
        for r in range(R):
            P.transpose(sts_p[0:1, r, :], iface[:, O_RS + r:O_RS + r + 1],
                        ident[0:T, 0:T])
        P.transpose(sts_p[0:1, 4, :], iface[:, O_WS:O_WS + 1], ident[0:T, 0:T])
        st_e = pp.tile([1, 5 * T], F32)
        S.activation(st_e[:], sts_p[:].rearrange("o f t -> o (f t)"), AF.Exp)
        st_w = pp.tile([1, 5 * T], F32)
        V.tensor_scalar(st_w[:], st_e[:], 1.0, None, op0=OP.add)
        st_sp = pp.tile([1, 5 * T], F32)
        S.activation(st_sp[:], st_w[:], AF.Ln)

        # key norms: rsqrt(sum keys^2) = exp(-0.5 ln)
        sqk = pp.tile([W, 5 * T], F32)
        S.activation(sqk[:], keys[:].rearrange("w f t -> w (f t)"), AF.Square)
        k2_p = psM.tile([1, 5 * T], F32, tag="wwb")
        P.matmul(k2_p[:], ones[0:W, 0:1], sqk[:])
        lk2 = pp.tile([1, 5 * T], F32)
        S.activation(lk2[:], k2_p[:], AF.Ln)
        kr = pp.tile([1, 5 * T], F32)
        S.activation(kr[:], lk2[:], AF.Exp, scale=-0.5)
        beta = pp.tile([1, 5 * T], F32)
        V.tensor_tensor(beta[:], st_sp[:], kr[:], op=OP.mult)
        kb_p = psM.tile([W, 5 * T], F32, tag="add")
        P.matmul(kb_p[:], ones[0:1, 0:W], beta[:])
        V.tensor_tensor(keysc[:].rearrange("w f t -> w (f t)"),
                        keys[:].rearrange("w f t -> w (f t)"), kb_p[:],
                        op=OP.mult)

        # gates: fg x4, ag, wg sigmoids
        gats_p = psS.tile([1, 6, T], F32, tag="s")
        for r in range(R):
            P.transpose(gats_p[0:1, r, :], iface[:, O_FG + r:O_FG + r + 1],
                        ident[0:T, 0:T])
        P.transpose(gats_p[0:1, 4, :], iface[:, O_AG:O_AG + 1], ident[0:T, 0:T])
        P.transpose(gats_p[0:1, 5, :], iface[:, O_WG:O_WG + 1], ident[0:T, 0:T])
        g_e = pp.tile([1, 6 * T], F32)
        S.activation(g_e[:], gats_p[:].rearrange("o g t -> o (g t)"), AF.Exp,
                     scale=-1.0)
        g_w = pp.tile([1, 6 * T], F32)
        V.tensor_scalar(g_w[:], g_e[:], 1.0, None, op0=OP.add)
        V.reciprocal(gr[:].rearrange("o g t -> o (g t)"), g_w[:])
        ag_t = gr[0:1, 4, :]
        wg_t = gr[0:1, 5, :]
        V.tensor_tensor(c1p[:], ag_t, wg_t, op=OP.mult)
        V.tensor_scalar(cn1[:], c1p[:], -1.0, None, op0=OP.mult)
        V.tensor_tensor(c2[:], wg_t, c1p[:], op=OP.subtract)

        # modes softmax -> rows per t
        me = pp.tile([T, 12], F32)
        S.activation(me[:], iface[:, O_MD:O_MD + 12], AF.Exp)
        me3 = me[:].rearrange("t (r m) -> t r m", m=3)
        msum = pp.tile([T, R], F32)
        V.tensor_tensor(msum[:], me3[:, :, 0], me3[:, :, 1], op=OP.add)
        V.tensor_tensor(msum[:], msum[:], me3[:, :, 2], op=OP.add)
        mrcp = pp.tile([T, R], F32)
        V.reciprocal(mrcp[:], msum[:])
        mn = pp.tile([T, 12], F32)
        mn3 = mn[:].rearrange("t (m r) -> t m r", r=R)
        me3b = me[:].rearrange("t (r m) -> t m r", m=3)
        for m in range(3):
            V.tensor_tensor(mn3[:, m, :], me3b[:, m, :], mrcp[:], op=OP.mult)
        # three m-blocks at base partition 0: modes0/1/2 [4, T]
        mblk_p = psS.tile([R, 3, T], F32, tag="s")
        for m in range(3):
            P.transpose(mblk_p[:, m, :], mn[:, 4 * m:4 * (m + 1)],
                        ident[0:T, 0:T])
        mblk = pp.tile([R, 3, T], F32)
        V.tensor_copy(mblk[:], mblk_p[:])
        m1sel_p = psS.tile([1, R, T], F32, tag="s")
        for r in range(R):
            P.matmul(m1sel_p[0:1, r, :], ident[0:R, r:r + 1], mblk[:, 1, :])
        V.tensor_copy(modes1[:].rearrange("o r t -> o (r t)"),
                      m1sel_p[:].rearrange("o r t -> o (r t)"))
        # flatten rows r of m-block 0/2 onto partition 0 via selector matmuls
        mrows_p = psS.tile([1, 2, R, T], F32, tag="s")
        for r in range(R):
            P.matmul(mrows_p[0:1, 0, r, :], ident[0:R, r:r + 1], mblk[:, 0, :])
            P.matmul(mrows_p[0:1, 1, r, :], ident[0:R, r:r + 1], mblk[:, 2, :])
        mrows = pp.tile([1, 2, R, T], F32)
        V.tensor_copy(mrows[:].rearrange("o a r t -> o (a r t)"),
                      mrows_p[:].rearrange("o a r t -> o (a r t)"))
        mb0_p = psB.tile([128, R * T], F32, tag="wb")
        P.matmul(mb0_p[:], ones[0:1, :], mrows[0:1, 0, :, :])
        V.tensor_copy(mbs0[:].rearrange("p r t -> p (r t)"), mb0_p[:])
        mb2_p = psB.tile([128, R * T], F32, tag="pb", bufs=2)
        P.matmul(mb2_p[:], ones[0:1, :], mrows[0:1, 1, :, :])
        V.tensor_copy(mbs2[:].rearrange("p r t -> p (r t)"), mb2_p[:])

    # ---------------- initial state ----------------
    mem_nrm = sp.tile([128, NT, W], F32, tag="mem_nrm")
    for c in range(NT):
        DMA.dma_start(mem_nrm[:, c, :],
                      m0_d[128 * c:128 * (c + 1), :])
    memT_p = psB.tile([W, N], F32, tag="wb")
    for c in range(NT):
        P.transpose(memT_p[:, 128 * c:128 * (c + 1)],
                    mem_nrm[:, c, :], ident[:])
    memT = sp.tile([W, N], F32, tag="memT")
    V.tensor_copy(memT[:], memT_p[:])

    # initial norm: PM-layout sqn -> msq -> Ln/Exp
    sqn0 = wp.tile([128, NT, W], F32, tag="sqn")
    G_.tensor_tensor(sqn0[:], mem_nrm[:], mem_nrm[:], op=OP.mult)
    msq0 = wp.tile([128, NT], F32, tag="msq")
    V.tensor_reduce(msq0[:], sqn0[:], axis=mybir.AxisListType.X, op=OP.add)
    lms0 = wp.tile([128, NT], F32, tag="lms")
    S.activation(lms0[:], msq0[:], AF.Ln)
    mnorm_i = sp.tile([128, NT], F32, tag="mnorm")
    S.activation(mnorm_i[:], lms0[:], AF.Exp, scale=-0.5)

    L = sp.tile([128, NT, N], BF16, tag="L")
    G_.memset(L[:], 0.0)
    LT0 = sp.tile([128, NT, N], BF16, tag="LT")
    G_.memset(LT0[:], 0.0)
    dL0 = sp.tile([128, NT], F32, tag="dL")
    G_.memset(dL0[:], 0.0)

    st = dict(memT=memT, mem_nrm=mem_nrm, mnorm=mnorm_i, L=L, LT=LT0,
              dL=dL0, u_pm=None, prec_pm=None, prec_fl=None,
              pbs=None, rwT=None)

    for t in range(T):
        st = _step(nc, t, st, cp, sp, wp, psB, psM, psS,
                   ident, ones, ones_b, iface, wvR, keysc, neg_er, gr, c1p,
                   cn1, c2, modes1, mbs0, mbs2, nege0_pm, out_sb)

    DMA.dma_start(out_d[:].rearrange("t r w -> w t r"), out_sb[:])


def _step(nc, t, st, cp, sp, wp, psB, psM, psS,
          ident, ones, ones_b, iface, wvR, keysc, neg_er, gr, c1p, cn1, c2,
          modes1, mbs0, mbs2, nege0_pm, out_sb):
    V, S, P, G_, DMA = nc.vector, nc.scalar, nc.tensor, nc.gpsimd, nc.sync
    memT, mem_nrm, mnorm = st["memT"], st["mem_nrm"], st["mnorm"]
    L, LT, dL, u_pm = st["L"], st["LT"], st["dL"], st["u_pm"]
    prec_pm, prec_fl, pbs, rwT = (st["prec_pm"], st["prec_fl"], st["pbs"],
                                  st["rwT"])
    last = (t == T - 1)

    if t == 0:
        na_pm, nsa = nege0_pm, None
    else:
        na_pm, nsa = st["na_pm"], st["nsa"]

    # ---- pbs broadcast (prev-step prec; runs at step start) ----
    if t > 0:
        pb_p = psB.tile([128, N], F32, tag="pb", bufs=2)
        P.matmul(pb_p[:], ones_b[0:1, :], prec_fl[:])
        pbs = wp.tile([128, N], BF16, tag="pbs")
        S.activation(pbs[:], pb_p[:], AF.Copy)

    # ---- write content softmax (PM) ----
    wdots_p = psS.tile([128, NT], F32, tag="s")
    for b in range(NT):
        P.matmul(wdots_p[:, b:b + 1], memT[:, 128 * b:128 * (b + 1)],
                 keysc[:, 4, t:t + 1])
    wlog = wp.tile([128, NT], F32, tag="wlog")
    V.tensor_tensor(wlog[:], wdots_p[:], mnorm[:], op=OP.mult)
    wexp_pm = wp.tile([128, NT], F32, tag="wexp")
    S.activation(wexp_pm[:], wlog[:], AF.Exp)
    wps_p = psS.tile([1, NT], F32, tag="s")
    P.matmul(wps_p[:], ones[:, 0:1], wexp_pm[:])
    wsum = wp.tile([1, 1], F32, tag="wsum")
    V.tensor_reduce(wsum[:], wps_p[:], axis=mybir.AxisListType.X, op=OP.add)
    wrs = wp.tile([1, 1], F32, tag="wrs")
    V.reciprocal(wrs[:], wsum[:])
    cw = wp.tile([1, 1], F32, tag="cw")
    V.tensor_tensor(cw[:], wrs[:], c2[0:1, t:t + 1], op=OP.mult)

    # ---- ww assembly (PM) ----
    cn1b_p = psS.tile([128, 1], F32, tag="s")
    P.matmul(cn1b_p[:], ones[0:1, :], cn1[0:1, t:t + 1])
    cwb_p = psS.tile([128, 1], F32, tag="s")
    P.matmul(cwb_p[:], ones[0:1, :], cw[:])
    wwx = wp.tile([128, NT], F32, tag="wwx")
    V.tensor_scalar(wwx[:], na_pm[:], cn1b_p[:, 0:1], None, op0=OP.mult)
    ww_pm = wp.tile([128, NT], F32, tag="wwpm")
    V.scalar_tensor_tensor(ww_pm[:], wexp_pm[:], cwb_p[:, 0:1], wwx[:],
                           op0=OP.mult, op1=OP.add)
    if t > 0:
        omw_pm = wp.tile([128, NT], F32, tag="omw")
        V.tensor_scalar(omw_pm[:], ww_pm[:], -1.0, 1.0, op0=OP.mult,
                        op1=OP.add)
    ww_tp = psS.tile([1, N], F32, tag="s")
    for c in range(NT):
        P.transpose(ww_tp[0:1, 128 * c:128 * (c + 1)], ww_pm[:, c:c + 1],
                    ident[:])
    ww_fl = wp.tile([1, N], BF16, tag="wwfl")
    S.copy(ww_fl[:], ww_tp[:])

    # ---- wbs broadcast (bf16, for the link ts ops) ----
    if t > 0:
        wb_p = psB.tile([128, N], F32, tag="wb")
        P.matmul(wb_p[:], ones_b[0:1, :], ww_fl[:])
        wbs = wp.tile([128, N], BF16, tag="wbs")
        S.activation(wbs[:], wb_p[:], AF.Copy)

    # ---- memory head ----
    wwb_p = psM.tile([W, N], F32, tag="wwb")
    P.matmul(wwb_p[:], ones_b[0:1, 0:W], ww_fl[:])
    add_p = psM.tile([W, N], F32, tag="add")
    P.matmul(add_p[:], wvR[0:1, t, :], ww_fl[:])
    keep = wp.tile([W, N], F32, tag="keep")
    S.activation(keep[:], wwb_p[:], AF.Copy, scale=neg_er[:, t:t + 1],
                 bias=1.0)
    m1 = wp.tile([W, N], F32, tag="m1")
    G_.tensor_tensor(m1[:], memT[:], keep[:], op=OP.mult)

    # ---- usage update ----
    if last:
        u_pm_n = u_pm
    else:
        u_pm_n = sp.tile([128, NT], F32, tag="u_pm")
        if t == 0:
            V.tensor_copy(u_pm_n[:], ww_pm[:])
        else:
            fgb_p = psS.tile([128, R], F32, tag="s")
            P.matmul(fgb_p[:], ones[0:1, :], gr[0:1, 0:R, t])
            yyT = wp.tile([128, NT, R], F32, tag="yyT")
            V.scalar_tensor_tensor(
                yyT[:], fgb_p[:, None, :].broadcast_to([128, NT, R]), -1.0,
                rwT[:].rearrange("p (c r) -> p c r", r=R),
                op0=OP.mult, op1=OP.mult)
            om = wp.tile([128, NT, R], F32, tag="om")
            V.tensor_scalar(om[:], yyT[:], 1.0, None, op0=OP.add)
            p1u = wp.tile([128, NT], F32, tag="p1u")
            G_.tensor_tensor(p1u[:], om[:, :, 0], om[:, :, 1], op=OP.mult)
            p2u = wp.tile([128, NT], F32, tag="p2u")
            G_.tensor_tensor(p2u[:], om[:, :, 2], om[:, :, 3], op=OP.mult)
            psi = wp.tile([128, NT], F32, tag="psi")
            G_.tensor_tensor(psi[:], p1u[:], p2u[:], op=OP.mult)
            omu = wp.tile([128, NT], F32, tag="omu")
            V.tensor_scalar(omu[:], u_pm[:], -1.0, 1.0, op0=OP.mult,
                            op1=OP.add)
            tn = wp.tile([128, NT], F32, tag="tn")
            V.scalar_tensor_tensor(tn[:], ww_pm[:], 1.0, omu[:],
                                   op0=OP.subtract, op1=OP.mult)
            V.scalar_tensor_tensor(u_pm_n[:], tn[:], 1.0, psi[:],
                                   op0=OP.add, op1=OP.mult)

    # ---- allocation compare inputs (flat u + broadcast; exact fp32) ----
    if not last:
        u_tp = psS.tile([1, N], F32, tag="s")
        for c in range(NT):
            P.transpose(u_tp[0:1, 128 * c:128 * (c + 1)], u_pm_n[:, c:c + 1],
                        ident[:])
        u_fl_n = wp.tile([1, N], F32, tag="ufl")
        V.tensor_copy(u_fl_n[:], u_tp[:])
        ub_p = psM.tile([128, N], F32, tag="wwb")
        P.matmul(ub_p[:], ones[0:1, :], u_fl_n[:])
        ubs = wp.tile([128, N], F32, tag="ubs")
        S.copy(ubs[:], ub_p[:])
        ucl = wp.tile([128, NT], F32, tag="ucl")
        V.tensor_scalar(ucl[:], u_pm_n[:], 1e-38, None, op0=OP.max)
        lnu = wp.tile([128, NT], F32, tag="lnu")
        S.activation(lnu[:], ucl[:], AF.Ln)

    # ---- prec update ----
    if not last:
        prec_pm_n = sp.tile([128, NT], F32, tag="prec_pm")
        if t == 0:
            V.tensor_copy(prec_pm_n[:], ww_pm[:])
        else:
            swa = wp.tile([1, 1], F32, tag="swa")
            G_.tensor_tensor(swa[:], nsa[:], cn1[0:1, t:t + 1], op=OP.mult)
            sw = wp.tile([1, 1], F32, tag="sw")
            G_.tensor_tensor(sw[:], swa[:], c2[0:1, t:t + 1], op=OP.add)
            omsw = wp.tile([1, 1], F32, tag="omsw")
            V.tensor_scalar(omsw[:], sw[:], -1.0, 1.0, op0=OP.mult,
                            op1=OP.add)
            omsw_p = psS.tile([128, 1], F32, tag="s")
            P.matmul(omsw_p[:], ones[0:1, :], omsw[:])
            V.scalar_tensor_tensor(prec_pm_n[:], prec_pm[:], omsw_p[:, 0:1],
                                   ww_pm[:], op0=OP.mult, op1=OP.add)
        p_tp = psS.tile([1, N], F32, tag="s")
        for c in range(NT):
            P.transpose(p_tp[0:1, 128 * c:128 * (c + 1)], prec_pm_n[:, c:c + 1],
                        ident[:])
        prec_fl_n = sp.tile([1, N], BF16, tag="prec_fl")
        S.copy(prec_fl_n[:], p_tp[:])
    else:
        prec_pm_n, prec_fl_n = prec_pm, prec_fl

    # ---- mode-scaled read weights + link diagonal tracker ----
    if t > 0:
        rwTm0 = wp.tile([128, NT * R], BF16, tag="rwTm0")
        V.tensor_tensor(rwTm0[:].rearrange("p (c r) -> p c r", r=R),
                        rwT[:].rearrange("p (c r) -> p c r", r=R),
                        mbs0[:, None, :, t].broadcast_to([128, NT, R]),
                        op=OP.mult)
        rwTm2 = wp.tile([128, NT * R], BF16, tag="rwTm2")
        V.tensor_tensor(rwTm2[:].rearrange("p (c r) -> p c r", r=R),
                        rwT[:].rearrange("p (c r) -> p c r", r=R),
                        mbs2[:, None, :, t].broadcast_to([128, NT, R]),
                        op=OP.mult)
        wp_pm = wp.tile([128, NT], F32, tag="wppm")
        G_.tensor_tensor(wp_pm[:], ww_pm[:], prec_pm[:], op=OP.mult)
        dmul = wp.tile([128, NT], F32, tag="dmul")
        V.tensor_scalar(dmul[:], ww_pm[:], -2.0, 1.0, op0=OP.mult, op1=OP.add)
        dL_n = sp.tile([128, NT], F32, tag="dL")
        V.scalar_tensor_tensor(dL_n[:], dL[:], 1.0, dmul[:],
                               op0=OP.mult, op1=OP.mult)
        G_.tensor_tensor(dL_n[:], dL_n[:], wp_pm[:], op=OP.add)
    else:
        dL_n = dL

    # ---- link loop with interleaved memT_n / Gt compares ----
    comb_eng = [(G_, G_), (G_, V), (G_, V), (G_, V)]
    if t == 0:
        L_n, LT_n = L, LT
        memT_n = sp.tile([W, N], F32, tag="memT")
        V.tensor_tensor(memT_n[:], m1[:], add_p[:], op=OP.add)
        if not last:
            Gt_n = wp.tile([128, NT, N], F32, tag="G", bufs=1)
            for c in range(NT):
                V.tensor_scalar(Gt_n[:, c, :], ubs[:], u_pm_n[:, c:c + 1],
                                None, op0=OP.is_gt)
    else:
        L_n = sp.tile([128, NT, N], BF16, tag="L")
        LT_n = sp.tile([128, NT, N], BF16, tag="LT")
        memT_n = sp.tile([W, N], F32, tag="memT")
        if not last:
            Gt_n = wp.tile([128, NT, N], F32, tag="G", bufs=1)
        for c in range(NT):
            w1 = wp.tile([128, N], BF16, tag=f"w1_{c % 2}")
            V.tensor_scalar(w1[:], wbs[:], omw_pm[:, c:c + 1], None,
                            op0=OP.subtract)
            p1 = wp.tile([128, N], BF16, tag=f"p1_{c % 2}")
            V.tensor_scalar(p1[:], pbs[:], ww_pm[:, c:c + 1], None,
                            op0=OP.mult)
            p1T = wp.tile([128, N], BF16, tag=f"p1T_{c % 2}")
            V.tensor_scalar(p1T[:], wbs[:], prec_pm[:, c:c + 1], None,
                            op0=OP.mult)
            t1 = wp.tile([128, N], BF16, tag=f"t1_{c % 2}")
            G_.tensor_tensor(t1[:], w1[:], L[:, c, :], op=OP.mult)
            t1T = wp.tile([128, N], BF16, tag=f"t1T_{c % 2}")
            G_.tensor_tensor(t1T[:], w1[:], LT[:, c, :], op=OP.mult)
            eL, eLT = comb_eng[c]
            eL.tensor_tensor(L_n[:, c, :], p1[:], t1[:], op=OP.subtract)
            eLT.tensor_tensor(LT_n[:, c, :], p1T[:], t1T[:], op=OP.subtract)
            if c == 0:
                V.tensor_tensor(memT_n[:], m1[:], add_p[:], op=OP.add)
            elif not last:
                cc = c - 1
                V.tensor_scalar(Gt_n[:, cc, :], ubs[:],
                                u_pm_n[:, cc:cc + 1], None, op0=OP.is_gt)
        if not last:
            V.tensor_scalar(Gt_n[:, 3, :], ubs[:], u_pm_n[:, 3:4],
                            None, op0=OP.is_gt)

    # ---- memory norm chain ----
    mem_nrm_p = psS.tile([128, NT, W], F32, tag="mn", bufs=1)
    for c in range(NT):
        P.transpose(mem_nrm_p[:, c, :], memT_n[:, 128 * c:128 * (c + 1)],
                    ident[0:W, 0:W])
    sqn = wp.tile([128, NT, W], F32, tag="sqn")
    S.activation(sqn[:], mem_nrm_p[:], AF.Square)
    msq = wp.tile([128, NT], F32, tag="msq")
    V.tensor_reduce(msq[:], sqn[:], axis=mybir.AxisListType.X, op=OP.add)
    mem_nrm_n = sp.tile([128, NT, W], F32, tag="mem_nrm")
    S.copy(mem_nrm_n[:], mem_nrm_p[:])
    lms = wp.tile([128, NT], F32, tag="lms")
    S.activation(lms[:], msq[:], AF.Ln)
    mnorm_n = sp.tile([128, NT], F32, tag="mnorm")
    S.activation(mnorm_n[:], lms[:], AF.Exp, scale=-0.5)

    # ---- allocation log-sum ----
    if last:
        na_pm_n, nsa_n = None, None
    else:
        sT_p = psS.tile([128, NT], F32, tag="s")
        for b in range(NT):
            for c in range(NT):
                P.matmul(sT_p[:, b:b + 1], Gt_n[:, c, 128 * b:128 * (b + 1)],
                         lnu[:, c:c + 1], start=(c == 0),
                         stop=(c == NT - 1))
        es_pm = wp.tile([128, NT], F32, tag="espm")
        S.activation(es_pm[:], sT_p[:], AF.Exp)
        na_pm_n = wp.tile([128, NT], F32, tag="napm")
        if t < T - 2:
            nap = wp.tile([128, 1], F32, tag="nap")
            V.scalar_tensor_tensor(na_pm_n[:], u_pm_n[:], 1.0, es_pm[:],
                                   op0=OP.subtract, op1=OP.mult,
                                   accum_out=nap[:])
        else:
            V.scalar_tensor_tensor(na_pm_n[:], u_pm_n[:], 1.0, es_pm[:],
                                   op0=OP.subtract, op1=OP.mult)
        if t < T - 2:
            nsa_p = psS.tile([1, 1], F32, tag="s")
            P.matmul(nsa_p[:], nap[:], ones[:, 0:1])
            nsa_n = wp.tile([1, 1], F32, tag="nsa")
            V.tensor_copy(nsa_n[:], nsa_p[:])
        else:
            nsa_n = None

    # ---- read content (PM) ----
    rdots_p = psS.tile([128, NT * R], F32, tag="s")
    for b in range(NT):
        P.matmul(rdots_p[:, R * b:R * (b + 1)],
                 memT_n[:, 128 * b:128 * (b + 1)], keysc[:, 0:4, t])
    rlog = wp.tile([128, NT, R], F32, tag="rlog")
    V.tensor_tensor(rlog[:],
                    rdots_p[:].rearrange("p (c r) -> p c r", r=R),
                    mnorm_n[:, :, None].broadcast_to([128, NT, R]),
                    op=OP.mult)
    rexp_pm = wp.tile([128, NT * R], F32, tag="rexp")
    S.activation(rexp_pm[:], rlog[:].rearrange("p c r -> p (c r)"), AF.Exp)
    rps_p = psS.tile([1, NT * R], F32, tag="s")
    P.matmul(rps_p[:], ones[:, 0:1], rexp_pm[:])
    rsum = wp.tile([1, R], F32, tag="rsum")
    V.tensor_reduce(rsum[:], rps_p[:].rearrange("o (c r) -> o r c", r=R),
                    axis=mybir.AxisListType.X, op=OP.add)
    rsr = wp.tile([1, R], F32, tag="rsr")
    V.reciprocal(rsr[:], rsum[:])
    s1c = wp.tile([1, R], F32, tag="s1c")
    V.tensor_tensor(s1c[:], rsr[:], modes1[0:1, :, t], op=OP.mult)
    s1cb_p = psS.tile([128, R], F32, tag="s")
    P.matmul(s1cb_p[:], ones[0:1, :], s1c[:])

    cnt = wp.tile([128, NT, R], F32, tag="cnt")
    V.tensor_tensor(cnt[:], rexp_pm[:].rearrange("p (c r) -> p c r", r=R),
                    s1cb_p[:, None, :].broadcast_to([128, NT, R]), op=OP.mult)
    rwT_n = sp.tile([128, NT * R], F32, tag="rwT")
    if t > 0:
        rwT_p = psS.tile([128, NT * R], F32, tag="s")
        for b in range(NT):
            blk = slice(128 * b, 128 * (b + 1))
            for c in range(NT):
                P.matmul(rwT_p[:, R * b:R * (b + 1)], L_n[:, c, blk],
                         rwTm0[:, R * c:R * (c + 1)],
                         start=(c == 0), stop=False)
            for c in range(NT):
                P.matmul(rwT_p[:, R * b:R * (b + 1)], LT_n[:, c, blk],
                         rwTm2[:, R * c:R * (c + 1)],
                         start=False, stop=(c == NT - 1))
        s02 = wp.tile([128, NT, R], F32, tag="s02")
        G_.tensor_tensor(s02[:], rwTm0[:].rearrange("p (c r) -> p c r", r=R),
                        rwTm2[:].rearrange("p (c r) -> p c r", r=R),
                        op=OP.add)
        corr = wp.tile([128, NT, R], F32, tag="corr")
        V.tensor_tensor(corr[:], s02[:],
                        dL_n[:, :, None].broadcast_to([128, NT, R]),
                        op=OP.mult)
        cnt2 = wp.tile([128, NT, R], F32, tag="cnt2")
        G_.tensor_tensor(cnt2[:], cnt[:], corr[:], op=OP.subtract)
        V.tensor_tensor(rwT_n[:], cnt2[:].rearrange("p c r -> p (c r)"),
                        rwT_p[:], op=OP.add)
    else:
        V.tensor_copy(rwT_n[:], cnt[:].rearrange("p c r -> p (c r)"))

    rwd_p = psS.tile([W, R], F32, tag="s")
    for c in range(NT):
        P.matmul(rwd_p[:], mem_nrm_n[:, c, :],
                 rwT_n[:, R * c:R * (c + 1)],
                 start=(c == 0), stop=(c == NT - 1))
    V.tensor_copy(out_sb[:, t, :], rwd_p[:])

    return dict(memT=memT_n, mem_nrm=mem_nrm_n, mnorm=mnorm_n, L=L_n,
                LT=LT_n, dL=dL_n, u_pm=u_pm_n, na_pm=na_pm_n,
                nsa=nsa_n, prec_pm=prec_pm_n, prec_fl=prec_fl_n, pbs=pbs,
                rwT=rwT_n)


# ---------------------------------------------------------------------------
_NC_CACHE = {}


def _get_nc():
    if "nc" not in _NC_CACHE:
        _NC_CACHE["nc"] = build_nc()
    return _NC_CACHE["nc"]


def _consts():
    ident = np.eye(128, dtype=np.float32)
    return (ident,)


def make_in_maps(controller_output, W_if, b_if, memory0):
    (ident,) = _consts()
    maps = []
    for b in range(B):
        maps.append({
            "co": np.ascontiguousarray(controller_output[b]),
            "wif": np.ascontiguousarray(W_if),
            "bif": np.ascontiguousarray(b_if.reshape(1, IF)),
            "mem0": np.ascontiguousarray(memory0[b]),
            "ident": ident,
        })
    return maps


def kernel(controller_output, W_if, b_if, memory0):
    from concourse.bass_utils import run_bass_kernel_spmd
    controller_output = np.asarray(controller_output, dtype=np.float32)
    W_if = np.asarray(W_if, dtype=np.float32)
    b_if = np.asarray(b_if, dtype=np.float32)
    memory0 = np.asarray(memory0, dtype=np.float32)
    nc = _get_nc()
    maps = make_in_maps(controller_output, W_if, b_if, memory0)
    res = run_bass_kernel_spmd(nc, maps, core_ids=list(range(B)))
    return np.stack([res.results[b]["out"] for b in range(B)], axis=0)


if __name__ == "__main__":
    mode = sys.argv[1] if len(sys.argv) > 1 else "sim"
    sys.path.insert(0, "/root/problem")
    import jax
    with jax.default_device(jax.devices("cpu")[0]):
        import reference
        inputs = {k: np.asarray(v) for k, v in reference.setup_inputs().items()}
        expected = np.asarray(reference.reference(**inputs))

    if mode == "sim":
        from concourse.bass_interp import CoreSim
        nc = build_nc()
        maps = make_in_maps(inputs["controller_output"], inputs["W_if"],
                            inputs["b_if"], inputs["memory0"])
        sim = CoreSim(nc)
        for k, v in maps[0].items():
            sim.tensor(k)[:] = v
        sim.simulate()
        got = sim.tensor("out").copy()
        exp = expected[0]
        err = np.abs(got - exp)
        rel = np.linalg.norm(got - exp) / (np.linalg.norm(exp) + 1e-12)
        print("sim modeled time (ns):", sim.time)
        print("max abs err:", err.max(), " rel err:", rel)
    else:
        got = kernel(**inputs)
        rel = np.linalg.norm(got - expected) / (np.linalg.norm(expected) + 1e-12)
        print("max abs err:", np.abs(got - expected).max(), " rel err:", rel)


# revision 11
# speedup vs baseline: 1.6960x; 1.0092x over previous
"""DNC MemoryAccess kernel for Trainium2 (Bass/Tile), data-parallel over batch.

Shapes (hardcoded): B=8, T=16, C=1024, IFACE=471, N=512, WORD=64, R=4, NW=1.
Each of the 8 cores processes one batch element; all recurrent state stays
SBUF-resident across the T=16 sequential steps.

Design (vs the fp32 predecessor, 326us -> 192us modeled):
- the temporal link matrix L and its transpose LT are held in bf16; the
  elementwise recurrence L' = (1-w_i-w_j)L + w_i p_j runs as fast-mode
  tensor_scalar ops (0.25x DVE cycles in bf16) for w1 = w_j-(1-w_i) and the
  rank-1 terms, with the tensor_tensor multiplies/combines split across
  Pool and DVE,
- the link diagonal is never zeroed in-place: the scalar recurrence
  d' = (1-2w)d + w p is tracked in [128,NT] and its contribution is
  subtracted from the fwd/bwd PE matmul results,
- broadcast matmuls (ww, prec over partitions/words) use bf16 operands
  (1 PE cycle/row vs 4 for fp32); the usage broadcast for the allocation
  sort compare stays exact fp32 so sort ties match the fp32 reference,
- ln(usage) for the allocation cumprod and the memory-norm rsqrt use the
  Activation-table Ln/Exp; get_activation_tables is patched (membership
  only, original set order preserved) so Exp and Ln resolve to the one
  act-func set that contains both, hoisting the 1.3us table load out of
  the step loop,
- emission order is tuned for the per-engine in-order queues: the read
  softmax, rwTm scaling and dL tracker are emitted so the DVE queue never
  head-blocks the ww chain of the next step; the precedence flat vector
  is produced by PE transposes + one Act copy,
- float32r matmuls are NOT used: they fail neuronxcc BIR verification in
  this toolchain (sim accepts them; hardware compile rejects).

Precision: bf16 rounds the link matrices and the write/erase broadcasts
(~1e-3 relative on the output); usage comparisons stay exact fp32 so the
allocation sort matches the reference except for genuine fp32 ties (b=7
carries one, same as the fp32 baseline).
"""
import sys

sys.path.insert(0, "/opt/trn_rl_repo")

import numpy as np

import concourse.bacc as bacc
import concourse.bass as bass
import concourse.mybir as mybir
import concourse.tile as tile

F32 = mybir.dt.float32
F32R = mybir.dt.float32r
BF16 = mybir.dt.bfloat16
I32 = mybir.dt.int32
AF = mybir.ActivationFunctionType
OP = mybir.AluOpType

B, T, C, IF = 8, 16, 1024, 471
N, W, R = 512, 64, 4
NT = N // 128

O_RK, O_RS, O_WK, O_WS = 0, 256, 260, 324
O_ER, O_WV, O_FG, O_AG, O_WG, O_MD = 325, 389, 453, 457, 458, 459


def fr(ap):
    return ap


# Prefer the activation-function set that contains Exp AND Ln (plus
# Copy/Square/Sign), so the per-step Exp/Ln mix resolves to one table and the
# compiler hoists a single LoadActFuncSet out of the step loop instead of
# thrashing 1283ns loads between exp-only and ln-only sets.
_ORIG_GET_ACT_TABLES = None


def _patch_act_tables():
    global _ORIG_GET_ACT_TABLES
    if _ORIG_GET_ACT_TABLES is not None:
        return
    import concourse.hw_specs as hw_specs
    _ORIG_GET_ACT_TABLES = hw_specs.get_activation_tables

    def pinned(arch):
        tabs = dict(_ORIG_GET_ACT_TABLES(arch))
        pref = "natural_log_exp_and_others"
        if pref not in tabs:
            return tabs
        exp_ln = {mybir.ActivationFunctionType.Exp,
                  mybir.ActivationFunctionType.Ln}
        out = {}
        for k, v in tabs.items():
            out[k] = set(v) if k == pref else set(v) - exp_ln
        return out

    bacc.get_activation_tables = pinned


def build_nc():
    _patch_act_tables()
    nc = bacc.Bacc("TRN2", target_bir_lowering=False, debug=False, num_devices=8)

    co_d = nc.declare_dram_parameter("co", [T, C], F32, isOutput=False)
    w_d = nc.declare_dram_parameter("wif", [C, IF], F32, isOutput=False)
    b_d = nc.declare_dram_parameter("bif", [1, IF], F32, isOutput=False)
    m0_d = nc.declare_dram_parameter("mem0", [N, W], F32, isOutput=False)
    ident_d = nc.declare_dram_parameter("ident", [128, 128], F32, isOutput=False)
    out_d = nc.declare_dram_parameter("out", [T, R, W], F32, isOutput=True)

    with tile.TileContext(nc) as tc:
        with (
            nc.allow_low_precision(reason="bf16 link + f32r broadcasts stay"
                                   " within the 2e-2 gate"),
            tc.tile_pool(name="const", bufs=1) as cp,
            tc.tile_pool(name="state", bufs=2) as sp,
            tc.tile_pool(name="work", bufs=2) as wp,
            tc.tile_pool(name="psBig", bufs=1, space="PSUM") as psB,
            tc.tile_pool(name="psMem", bufs=1, space="PSUM") as psM,
            tc.tile_pool(name="psS", bufs=2, space="PSUM") as psS,
        ):
            _build_body(nc, tc, cp, sp, wp, psB, psM, psS,
                        co_d, w_d, b_d, m0_d, ident_d, out_d)
    nc.compile()
    return nc


def _build_body(nc, tc, cp, sp, wp, psB, psM, psS,
                co_d, w_d, b_d, m0_d, ident_d, out_d):
    V, S, P, G_, DMA = nc.vector, nc.scalar, nc.tensor, nc.gpsimd, nc.sync

    # ---------------- constants ----------------
    ident = cp.tile([128, 128], F32)
    DMA.dma_start(ident[:], ident_d[:])
    ones = cp.tile([128, 128], F32)
    G_.memset(ones[:], 1.0)
    ones_b = cp.tile([1, 128], BF16)
    G_.memset(ones_b[:], 1.0)

    # persistent per-t tables
    iface = cp.tile([T, IF], F32)          # raw iface rows
    wvR = cp.tile([1, T, W], BF16)         # write vectors, partition-0 rows
    keysc = cp.tile([W, 5, T], F32)        # scaled keys: r=0..3 read, 4 write
    neg_er = cp.tile([W, T], F32)
    gr = cp.tile([1, 6, T], F32)           # sigmoids: fg x4, ag, wg
    c1p = cp.tile([1, T], F32)
    cn1 = cp.tile([1, T], F32)
    c2 = cp.tile([1, T], F32)
    modes1 = cp.tile([1, R, T], F32)       # content-mode row per t
    mbs0 = cp.tile([128, R, T], F32)
    mbs2 = cp.tile([128, R, T], F32)
    nege0_pm = cp.tile([128, NT], F32)
    G_.memset(nege0_pm[:], 0.0)
    G_.memset(nege0_pm[0:1, 0:1], -1.0)
    out_sb = cp.tile([W, T, R], F32)

    # ---------------- prologue ----------------
    with tc.tile_pool(name="prolog", bufs=1) as pp:
        co_sb = pp.tile([T, C], F32)
        DMA.dma_start(co_sb[:], co_d[:])
        bif_sb = pp.tile([1, IF], F32)
        DMA.dma_start(bif_sb[:], b_d[:])
        w_sb = pp.tile([128, 8, IF], F32)
        for k in range(8):
            # split the 1.9MB load across two hwdge queues
            eng = DMA if k % 2 == 0 else nc.scalar
            eng.dma_start(w_sb[:, k, :], w_d[128 * k:128 * (k + 1), :])

        coT_p = psB.tile([128, 8, T], F32, tag="wb")
        for k in range(8):
            P.transpose(coT_p[:, k, :], co_sb[:, 128 * k:128 * (k + 1)],
                        ident[0:T, 0:T])
        coT = pp.tile([128, 8, T], F32)
        V.tensor_copy(coT[:], coT_p[:])

        if_p = psB.tile([T, IF], F32, tag="pb", bufs=2)
        for k in range(8):
            P.matmul(if_p[:], coT[:, k, :], w_sb[:, k, :],
                     start=(k == 0), stop=False)
        P.matmul(if_p[:], ones[0:1, 0:T], bif_sb[:],
                 start=False, stop=True)
        V.tensor_copy(iface[:], if_p[:])

        # keys [64, 5, T]: read r=0..3, write at 4
        keys_p = psB.tile([W, 5, T], F32, tag="pb", bufs=2)
        for r in range(R):
            P.transpose(keys_p[:, r, :], iface[:, O_RK + W * r:O_RK + W * (r + 1)],
                        ident[0:T, 0:T])
        P.transpose(keys_p[:, 4, :], iface[:, O_WK:O_WK + W], ident[0:T, 0:T])
        keys = pp.tile([W, 5, T], F32)
        V.tensor_copy(keys[:], keys_p[:])

        # write vectors as partition-0 rows via selector matmuls, two copies
        for h in range(2):
            wv_p = psB.tile([1, 8, W], F32, tag="pb", bufs=2, name=f"wvp{h}")
            for j in range(8):
                tt_ = 8 * h + j
                P.matmul(wv_p[0:1, j, :], ident[0:T, tt_:tt_ + 1],
                         iface[:, O_WV:O_WV + W])
            V.tensor_copy(wvR[0:1, 8 * h:8 * (h + 1), :].rearrange(
                "o t w -> o (t w)"),
                wv_p[:].rearrange("o t w -> o (t w)"))

        # erase sigmoid -> neg_er
        er_p = psS.tile([W, T], F32, tag="s")
        P.transpose(er_p[:], iface[:, O_ER:O_ER + W], ident[0:T, 0:T])
        ee = pp.tile([W, T], F32)
        S.activation(ee[:], er_p[:], AF.Exp, scale=-1.0)
        ew = pp.tile([W, T], F32)
        V.tensor_scalar(ew[:], ee[:], 1.0, None, op0=OP.add)
        er_r = pp.tile([W, T], F32)
        V.reciprocal(er_r[:], ew[:])
        V.tensor_scalar(neg_er[:], er_r[:], -1.0, None, op0=OP.mult)

        # strengths softplus: [1, 5, T] (rs x4, ws)
        sts_p = psS.tile([1, 5, T], F32, tag="s")
        for r in range(R):
            P.transpose(sts_p[0:1, r, :], iface[:, O_RS + r:O_RS + r + 1],
                        ident[0:T, 0:T])
        P.transpose(sts_p[0:1, 4, :], iface[:, O_WS:O_WS + 1], ident[0:T, 0:T])
        st_e = pp.tile([1, 5 * T], F32)
        S.activation(st_e[:], sts_p[:].rearrange("o f t -> o (f t)"), AF.Exp)
        st_w = pp.tile([1, 5 * T], F32)
        V.tensor_scalar(st_w[:], st_e[:], 1.0, None, op0=OP.add)
        st_sp = pp.tile([1, 5 * T], F32)
        S.activation(st_sp[:], st_w[:], AF.Ln)

        # key norms: rsqrt(sum keys^2) = exp(-0.5 ln)
        sqk = pp.tile([W, 5 * T], F32)
        S.activation(sqk[:], keys[:].rearrange("w f t -> w (f t)"), AF.Square)
        k2_p = psM.tile([1, 5 * T], F32, tag="wwb")
        P.matmul(k2_p[:], ones[0:W, 0:1], sqk[:])
        lk2 = pp.tile([1, 5 * T], F32)
        S.activation(lk2[:], k2_p[:], AF.Ln)
        kr = pp.tile([1, 5 * T], F32)
        S.activation(kr[:], lk2[:], AF.Exp, scale=-0.5)
        beta = pp.tile([1, 5 * T], F32)
        V.tensor_tensor(beta[:], st_sp[:], kr[:], op=OP.mult)
        kb_p = psM.tile([W, 5 * T], F32, tag="add")
        P.matmul(kb_p[:], ones[0:1, 0:W], beta[:])
        V.tensor_tensor(keysc[:].rearrange("w f t -> w (f t)"),
                        keys[:].rearrange("w f t -> w (f t)"), kb_p[:],
                        op=OP.mult)

        # gates: fg x4, ag, wg sigmoids
        gats_p = psS.tile([1, 6, T], F32, tag="s")
        for r in range(R):
            P.transpose(gats_p[0:1, r, :], iface[:, O_FG + r:O_FG + r + 1],
                        ident[0:T, 0:T])
        P.transpose(gats_p[0:1, 4, :], iface[:, O_AG:O_AG + 1], ident[0:T, 0:T])
        P.transpose(gats_p[0:1, 5, :], iface[:, O_WG:O_WG + 1], ident[0:T, 0:T])
        g_e = pp.tile([1, 6 * T], F32)
        S.activation(g_e[:], gats_p[:].rearrange("o g t -> o (g t)"), AF.Exp,
                     scale=-1.0)
        g_w = pp.tile([1, 6 * T], F32)
        V.tensor_scalar(g_w[:], g_e[:], 1.0, None, op0=OP.add)
        V.reciprocal(gr[:].rearrange("o g t -> o (g t)"), g_w[:])
        ag_t = gr[0:1, 4, :]
        wg_t = gr[0:1, 5, :]
        V.tensor_tensor(c1p[:], ag_t, wg_t, op=OP.mult)
        V.tensor_scalar(cn1[:], c1p[:], -1.0, None, op0=OP.mult)
        V.tensor_tensor(c2[:], wg_t, c1p[:], op=OP.subtract)

        # modes softmax -> rows per t
        me = pp.tile([T, 12], F32)
        S.activation(me[:], iface[:, O_MD:O_MD + 12], AF.Exp)
        me3 = me[:].rearrange("t (r m) -> t r m", m=3)
        msum = pp.tile([T, R], F32)
        V.tensor_tensor(msum[:], me3[:, :, 0], me3[:, :, 1], op=OP.add)
        V.tensor_tensor(msum[:], msum[:], me3[:, :, 2], op=OP.add)
        mrcp = pp.tile([T, R], F32)
        V.reciprocal(mrcp[:], msum[:])
        mn = pp.tile([T, 12], F32)
        mn3 = mn[:].rearrange("t (m r) -> t m r", r=R)
        me3b = me[:].rearrange("t (r m) -> t m r", m=3)
        for m in range(3):
            V.tensor_tensor(mn3[:, m, :], me3b[:, m, :], mrcp[:], op=OP.mult)
        # three m-blocks at base partition 0: modes0/1/2 [4, T]
        mblk_p = psS.tile([R, 3, T], F32, tag="s")
        for m in range(3):
            P.transpose(mblk_p[:, m, :], mn[:, 4 * m:4 * (m + 1)],
                        ident[0:T, 0:T])
        mblk = pp.tile([R, 3, T], F32)
        V.tensor_copy(mblk[:], mblk_p[:])
        m1sel_p = psS.tile([1, R, T], F32, tag="s")
        for r in range(R):
            P.matmul(m1sel_p[0:1, r, :], ident[0:R, r:r + 1], mblk[:, 1, :])
        V.tensor_copy(modes1[:].rearrange("o r t -> o (r t)"),
                      m1sel_p[:].rearrange("o r t -> o (r t)"))
        # flatten rows r of m-block 0/2 onto partition 0 via selector matmuls
        mrows_p = psS.tile([1, 2, R, T], F32, tag="s")
        for r in range(R):
            P.matmul(mrows_p[0:1, 0, r, :], ident[0:R, r:r + 1], mblk[:, 0, :])
            P.matmul(mrows_p[0:1, 1, r, :], ident[0:R, r:r + 1], mblk[:, 2, :])
        mrows = pp.tile([1, 2, R, T], F32)
        V.tensor_copy(mrows[:].rearrange("o a r t -> o (a r t)"),
                      mrows_p[:].rearrange("o a r t -> o (a r t)"))
        mb0_p = psB.tile([128, R * T], F32, tag="wb")
        P.matmul(mb0_p[:], ones[0:1, :], mrows[0:1, 0, :, :])
        V.tensor_copy(mbs0[:].rearrange("p r t -> p (r t)"), mb0_p[:])
        mb2_p = psB.tile([128, R * T], F32, tag="pb", bufs=2)
        P.matmul(mb2_p[:], ones[0:1, :], mrows[0:1, 1, :, :])
        V.tensor_copy(mbs2[:].rearrange("p r t -> p (r t)"), mb2_p[:])

    # ---------------- initial state ----------------
    mem_nrm = sp.tile([128, NT, W], F32, tag="mem_nrm")
    for c in range(NT):
        DMA.dma_start(mem_nrm[:, c, :],
                      m0_d[128 * c:128 * (c + 1), :])
    memT_p = psB.tile([W, N], F32, tag="wb")
    for c in range(NT):
        P.transpose(memT_p[:, 128 * c:128 * (c + 1)],
                    mem_nrm[:, c, :], ident[:])
    memT = sp.tile([W, N], F32, tag="memT")
    V.tensor_copy(memT[:], memT_p[:])

    # initial norm: PM-layout sqn -> msq -> Ln/Exp
    sqn0 = wp.tile([128, NT, W], F32, tag="sqn")
    G_.tensor_tensor(sqn0[:], mem_nrm[:], mem_nrm[:], op=OP.mult)
    msq0 = wp.tile([128, NT], F32, tag="msq")
    V.tensor_reduce(msq0[:], sqn0[:], axis=mybir.AxisListType.X, op=OP.add)
    lms0 = wp.tile([128, NT], F32, tag="lms")
    S.activation(lms0[:], msq0[:], AF.Ln)
    mnorm_i = sp.tile([128, NT], F32, tag="mnorm")
    S.activation(mnorm_i[:], lms0[:], AF.Exp, scale=-0.5)

    L = sp.tile([128, NT, N], BF16, tag="L")
    G_.memset(L[:], 0.0)
    LT0 = sp.tile([128, NT, N], BF16, tag="LT")
    G_.memset(LT0[:], 0.0)
    dL0 = sp.tile([128, NT], F32, tag="dL")
    G_.memset(dL0[:], 0.0)

    st = dict(memT=memT, mem_nrm=mem_nrm, mnorm=mnorm_i, L=L, LT=LT0,
              dL=dL0, u_pm=None, prec_pm=None, prec_fl=None,
              pbs=None, rwT=None)

    for t in range(T):
        st = _step(nc, t, st, cp, sp, wp, psB, psM, psS,
                   ident, ones, ones_b, iface, wvR, keysc, neg_er, gr, c1p,
                   cn1, c2, modes1, mbs0, mbs2, nege0_pm, out_sb)

    DMA.dma_start(out_d[:].rearrange("t r w -> w t r"), out_sb[:])


def _step(nc, t, st, cp, sp, wp, psB, psM, psS,
          ident, ones, ones_b, iface, wvR, keysc, neg_er, gr, c1p, cn1, c2,
          modes1, mbs0, mbs2, nege0_pm, out_sb):
    V, S, P, G_, DMA = nc.vector, nc.scalar, nc.tensor, nc.gpsimd, nc.sync
    memT, mem_nrm, mnorm = st["memT"], st["mem_nrm"], st["mnorm"]
    L, LT, dL, u_pm = st["L"], st["LT"], st["dL"], st["u_pm"]
    prec_pm, prec_fl, pbs, rwT = (st["prec_pm"], st["prec_fl"], st["pbs"],
                                  st["rwT"])
    last = (t == T - 1)

    if t == 0:
        na_pm, nsa = nege0_pm, None
    else:
        na_pm, nsa = st["na_pm"], st["nsa"]

    # ---- pbs broadcast (prev-step prec; runs at step start) ----
    if t > 0:
        pb_p = psB.tile([128, N], F32, tag="pb", bufs=2)
        P.matmul(pb_p[:], ones_b[0:1, :], prec_fl[:])
        pbs = wp.tile([128, N], BF16, tag="pbs")
        S.activation(pbs[:], pb_p[:], AF.Copy)

    # ---- write content softmax (PM) ----
    wdots_p = psS.tile([128, NT], F32, tag="s")
    for b in range(NT):
        P.matmul(wdots_p[:, b:b + 1], memT[:, 128 * b:128 * (b + 1)],
                 keysc[:, 4, t:t + 1])
    wlog = wp.tile([128, NT], F32, tag="wlog")
    V.tensor_tensor(wlog[:], wdots_p[:], mnorm[:], op=OP.mult)
    wexp_pm = wp.tile([128, NT], F32, tag="wexp")
    S.activation(wexp_pm[:], wlog[:], AF.Exp)
    wps_p = psS.tile([1, NT], F32, tag="s")
    P.matmul(wps_p[:], ones[:, 0:1], wexp_pm[:])
    wsum = wp.tile([1, 1], F32, tag="wsum")
    V.tensor_reduce(wsum[:], wps_p[:], axis=mybir.AxisListType.X, op=OP.add)
    wrs = wp.tile([1, 1], F32, tag="wrs")
    V.reciprocal(wrs[:], wsum[:])
    cw = wp.tile([1, 1], F32, tag="cw")
    V.tensor_tensor(cw[:], wrs[:], c2[0:1, t:t + 1], op=OP.mult)

    # ---- ww assembly (PM) ----
    cn1b_p = psS.tile([128, 1], F32, tag="s")
    P.matmul(cn1b_p[:], ones[0:1, :], cn1[0:1, t:t + 1])
    cwb_p = psS.tile([128, 1], F32, tag="s")
    P.matmul(cwb_p[:], ones[0:1, :], cw[:])
    wwx = wp.tile([128, NT], F32, tag="wwx")
    V.tensor_scalar(wwx[:], na_pm[:], cn1b_p[:, 0:1], None, op0=OP.mult)
    ww_pm = wp.tile([128, NT], F32, tag="wwpm")
    V.scalar_tensor_tensor(ww_pm[:], wexp_pm[:], cwb_p[:, 0:1], wwx[:],
                           op0=OP.mult, op1=OP.add)
    if t > 0:
        omw_pm = wp.tile([128, NT], F32, tag="omw")
        V.tensor_scalar(omw_pm[:], ww_pm[:], -1.0, 1.0, op0=OP.mult,
                        op1=OP.add)
    ww_tp = psS.tile([1, N], F32, tag="s")
    for c in range(NT):
        P.transpose(ww_tp[0:1, 128 * c:128 * (c + 1)], ww_pm[:, c:c + 1],
                    ident[:])
    ww_fl = wp.tile([1, N], BF16, tag="wwfl")
    S.copy(ww_fl[:], ww_tp[:])

    # ---- wbs broadcast (bf16, for the link ts ops) ----
    if t > 0:
        wb_p = psB.tile([128, N], F32, tag="wb")
        P.matmul(wb_p[:], ones_b[0:1, :], ww_fl[:])
        wbs = wp.tile([128, N], BF16, tag="wbs")
        S.activation(wbs[:], wb_p[:], AF.Copy)

    # ---- memory head ----
    wwb_p = psM.tile([W, N], F32, tag="wwb")
    P.matmul(wwb_p[:], ones_b[0:1, 0:W], ww_fl[:])
    add_p = psM.tile([W, N], F32, tag="add")
    P.matmul(add_p[:], wvR[0:1, t, :], ww_fl[:])
    keep = wp.tile([W, N], F32, tag="keep")
    S.activation(keep[:], wwb_p[:], AF.Copy, scale=neg_er[:, t:t + 1],
                 bias=1.0)
    m1 = wp.tile([W, N], F32, tag="m1")
    G_.tensor_tensor(m1[:], memT[:], keep[:], op=OP.mult)

    # ---- usage update ----
    if last:
        u_pm_n = u_pm
    else:
        u_pm_n = sp.tile([128, NT], F32, tag="u_pm")
        if t == 0:
            V.tensor_copy(u_pm_n[:], ww_pm[:])
        else:
            fgb_p = psS.tile([128, R], F32, tag="s")
            P.matmul(fgb_p[:], ones[0:1, :], gr[0:1, 0:R, t])
            yyT = wp.tile([128, NT, R], F32, tag="yyT")
            V.scalar_tensor_tensor(
                yyT[:], fgb_p[:, None, :].broadcast_to([128, NT, R]), -1.0,
                rwT[:].rearrange("p (c r) -> p c r", r=R),
                op0=OP.mult, op1=OP.mult)
            om = wp.tile([128, NT, R], F32, tag="om")
            V.tensor_scalar(om[:], yyT[:], 1.0, None, op0=OP.add)
            p1u = wp.tile([128, NT], F32, tag="p1u")
            G_.tensor_tensor(p1u[:], om[:, :, 0], om[:, :, 1], op=OP.mult)
            p2u = wp.tile([128, NT], F32, tag="p2u")
            G_.tensor_tensor(p2u[:], om[:, :, 2], om[:, :, 3], op=OP.mult)
            psi = wp.tile([128, NT], F32, tag="psi")
            G_.tensor_tensor(psi[:], p1u[:], p2u[:], op=OP.mult)
            omu = wp.tile([128, NT], F32, tag="omu")
            V.tensor_scalar(omu[:], u_pm[:], -1.0, 1.0, op0=OP.mult,
                            op1=OP.add)
            tn = wp.tile([128, NT], F32, tag="tn")
            V.scalar_tensor_tensor(tn[:], ww_pm[:], 1.0, omu[:],
                                   op0=OP.subtract, op1=OP.mult)
            V.scalar_tensor_tensor(u_pm_n[:], tn[:], 1.0, psi[:],
                                   op0=OP.add, op1=OP.mult)

    # ---- allocation compare inputs (flat u + broadcast; exact fp32) ----
    if not last:
        u_tp = psS.tile([1, N], F32, tag="s")
        for c in range(NT):
            P.transpose(u_tp[0:1, 128 * c:128 * (c + 1)], u_pm_n[:, c:c + 1],
                        ident[:])
        u_fl_n = wp.tile([1, N], F32, tag="ufl")
        V.tensor_copy(u_fl_n[:], u_tp[:])
        ub_p = psM.tile([128, N], F32, tag="wwb")
        P.matmul(ub_p[:], ones[0:1, :], u_fl_n[:])
        ubs = wp.tile([128, N], F32, tag="ubs")
        S.copy(ubs[:], ub_p[:])
        ucl = wp.tile([128, NT], F32, tag="ucl")
        V.tensor_scalar(ucl[:], u_pm_n[:], 1e-38, None, op0=OP.max)
        lnu = wp.tile([128, NT], F32, tag="lnu")
        S.activation(lnu[:], ucl[:], AF.Ln)

    # ---- prec update ----
    if not last:
        prec_pm_n = sp.tile([128, NT], F32, tag="prec_pm")
        if t == 0:
            V.tensor_copy(prec_pm_n[:], ww_pm[:])
        else:
            swa = wp.tile([1, 1], F32, tag="swa")
            G_.tensor_tensor(swa[:], nsa[:], cn1[0:1, t:t + 1], op=OP.mult)
            sw = wp.tile([1, 1], F32, tag="sw")
            G_.tensor_tensor(sw[:], swa[:], c2[0:1, t:t + 1], op=OP.add)
            omsw = wp.tile([1, 1], F32, tag="omsw")
            V.tensor_scalar(omsw[:], sw[:], -1.0, 1.0, op0=OP.mult,
                            op1=OP.add)
            omsw_p = psS.tile([128, 1], F32, tag="s")
            P.matmul(omsw_p[:], ones[0:1, :], omsw[:])
            V.scalar_tensor_tensor(prec_pm_n[:], prec_pm[:], omsw_p[:, 0:1],
                                   ww_pm[:], op0=OP.mult, op1=OP.add)
        p_tp = psS.tile([1, N], F32, tag="s")
        for c in range(NT):
            P.transpose(p_tp[0:1, 128 * c:128 * (c + 1)], prec_pm_n[:, c:c + 1],
                        ident[:])
        prec_fl_n = sp.tile([1, N], BF16, tag="prec_fl")
        S.copy(prec_fl_n[:], p_tp[:])
    else:
        prec_pm_n, prec_fl_n = prec_pm, prec_fl

    # ---- mode-scaled read weights + link diagonal tracker ----

    # ---- link loop with interleaved memT_n / Gt compares ----
    comb_eng = [(G_, G_), (G_, V), (G_, V), (G_, V)]
    if t == 0:
        L_n, LT_n = L, LT
        memT_n = sp.tile([W, N], F32, tag="memT")
        V.tensor_tensor(memT_n[:], m1[:], add_p[:], op=OP.add)
        if not last:
            Gt_n = wp.tile([128, NT, N], F32, tag="G", bufs=1)
            for c in range(NT):
                V.tensor_scalar(Gt_n[:, c, :], ubs[:], u_pm_n[:, c:c + 1],
                                None, op0=OP.is_gt)
    else:
        L_n = sp.tile([128, NT, N], BF16, tag="L")
        LT_n = sp.tile([128, NT, N], BF16, tag="LT")
        memT_n = sp.tile([W, N], F32, tag="memT")
        if not last:
            Gt_n = wp.tile([128, NT, N], F32, tag="G", bufs=1)
        for c in range(NT):
            w1 = wp.tile([128, N], BF16, tag=f"w1_{c % 2}")
            V.tensor_scalar(w1[:], wbs[:], omw_pm[:, c:c + 1], None,
                            op0=OP.subtract)
            p1 = wp.tile([128, N], BF16, tag=f"p1_{c % 2}")
            V.tensor_scalar(p1[:], pbs[:], ww_pm[:, c:c + 1], None,
                            op0=OP.mult)
            p1T = wp.tile([128, N], BF16, tag=f"p1T_{c % 2}")
            V.tensor_scalar(p1T[:], wbs[:], prec_pm[:, c:c + 1], None,
                            op0=OP.mult)
            t1 = wp.tile([128, N], BF16, tag=f"t1_{c % 2}")
            G_.tensor_tensor(t1[:], w1[:], L[:, c, :], op=OP.mult)
            t1T = wp.tile([128, N], BF16, tag=f"t1T_{c % 2}")
            G_.tensor_tensor(t1T[:], w1[:], LT[:, c, :], op=OP.mult)
            eL, eLT = comb_eng[c]
            eL.tensor_tensor(L_n[:, c, :], p1[:], t1[:], op=OP.subtract)
            eLT.tensor_tensor(LT_n[:, c, :], p1T[:], t1T[:], op=OP.subtract)
            if c == 0:
                V.tensor_tensor(memT_n[:], m1[:], add_p[:], op=OP.add)
            elif not last:
                cc = c - 1
                V.tensor_scalar(Gt_n[:, cc, :], ubs[:],
                                u_pm_n[:, cc:cc + 1], None, op0=OP.is_gt)
        if not last:
            V.tensor_scalar(Gt_n[:, 3, :], ubs[:], u_pm_n[:, 3:4],
                            None, op0=OP.is_gt)

    if t > 0:
        wp_pm = wp.tile([128, NT], F32, tag="wppm")
        G_.tensor_tensor(wp_pm[:], ww_pm[:], prec_pm[:], op=OP.mult)
        dmul = wp.tile([128, NT], F32, tag="dmul")
        V.tensor_scalar(dmul[:], ww_pm[:], -2.0, 1.0, op0=OP.mult, op1=OP.add)
        dL_n = sp.tile([128, NT], F32, tag="dL")
        V.scalar_tensor_tensor(dL_n[:], dL[:], 1.0, dmul[:],
                               op0=OP.mult, op1=OP.mult)
        G_.tensor_tensor(dL_n[:], dL_n[:], wp_pm[:], op=OP.add)
    else:
        dL_n = dL
    if t > 0:
        rwTm0 = wp.tile([128, NT * R], BF16, tag="rwTm0")
        V.tensor_tensor(rwTm0[:].rearrange("p (c r) -> p c r", r=R),
                        rwT[:].rearrange("p (c r) -> p c r", r=R),
                        mbs0[:, None, :, t].broadcast_to([128, NT, R]),
                        op=OP.mult)
        rwTm2 = wp.tile([128, NT * R], BF16, tag="rwTm2")
        V.tensor_tensor(rwTm2[:].rearrange("p (c r) -> p c r", r=R),
                        rwT[:].rearrange("p (c r) -> p c r", r=R),
                        mbs2[:, None, :, t].broadcast_to([128, NT, R]),
                        op=OP.mult)

    # ---- memory norm chain ----
    mem_nrm_p = psS.tile([128, NT, W], F32, tag="mn", bufs=1)
    for c in range(NT):
        P.transpose(mem_nrm_p[:, c, :], memT_n[:, 128 * c:128 * (c + 1)],
                    ident[0:W, 0:W])
    sqn = wp.tile([128, NT, W], F32, tag="sqn")
    S.activation(sqn[:], mem_nrm_p[:], AF.Square)
    msq = wp.tile([128, NT], F32, tag="msq")
    V.tensor_reduce(msq[:], sqn[:], axis=mybir.AxisListType.X, op=OP.add)
    mem_nrm_n = sp.tile([128, NT, W], F32, tag="mem_nrm")
    S.copy(mem_nrm_n[:], mem_nrm_p[:])
    lms = wp.tile([128, NT], F32, tag="lms")
    S.activation(lms[:], msq[:], AF.Ln)
    mnorm_n = sp.tile([128, NT], F32, tag="mnorm")
    S.activation(mnorm_n[:], lms[:], AF.Exp, scale=-0.5)

    # ---- allocation log-sum ----
    if last:
        na_pm_n, nsa_n = None, None
    else:
        sT_p = psS.tile([128, NT], F32, tag="s")
        for b in range(NT):
            for c in range(NT):
                P.matmul(sT_p[:, b:b + 1], Gt_n[:, c, 128 * b:128 * (b + 1)],
                         lnu[:, c:c + 1], start=(c == 0),
                         stop=(c == NT - 1))
        es_pm = wp.tile([128, NT], F32, tag="espm")
        S.activation(es_pm[:], sT_p[:], AF.Exp)
        na_pm_n = wp.tile([128, NT], F32, tag="napm")
        if t < T - 2:
            nap = wp.tile([128, 1], F32, tag="nap")
            V.scalar_tensor_tensor(na_pm_n[:], u_pm_n[:], 1.0, es_pm[:],
                                   op0=OP.subtract, op1=OP.mult,
                                   accum_out=nap[:])
        else:
            V.scalar_tensor_tensor(na_pm_n[:], u_pm_n[:], 1.0, es_pm[:],
                                   op0=OP.subtract, op1=OP.mult)
        if t < T - 2:
            nsa_p = psS.tile([1, 1], F32, tag="s")
            P.matmul(nsa_p[:], nap[:], ones[:, 0:1])
            nsa_n = wp.tile([1, 1], F32, tag="nsa")
            V.tensor_copy(nsa_n[:], nsa_p[:])
        else:
            nsa_n = None

    # ---- read content (PM) ----
    rdots_p = psS.tile([128, NT * R], F32, tag="s")
    for b in range(NT):
        P.matmul(rdots_p[:, R * b:R * (b + 1)],
                 memT_n[:, 128 * b:128 * (b + 1)], keysc[:, 0:4, t])
    rlog = wp.tile([128, NT, R], F32, tag="rlog")
    V.tensor_tensor(rlog[:],
                    rdots_p[:].rearrange("p (c r) -> p c r", r=R),
                    mnorm_n[:, :, None].broadcast_to([128, NT, R]),
                    op=OP.mult)
    rexp_pm = wp.tile([128, NT * R], F32, tag="rexp")
    S.activation(rexp_pm[:], rlog[:].rearrange("p c r -> p (c r)"), AF.Exp)
    rps_p = psS.tile([1, NT * R], F32, tag="s")
    P.matmul(rps_p[:], ones[:, 0:1], rexp_pm[:])
    rsum = wp.tile([1, R], F32, tag="rsum")
    V.tensor_reduce(rsum[:], rps_p[:].rearrange("o (c r) -> o r c", r=R),
                    axis=mybir.AxisListType.X, op=OP.add)
    rsr = wp.tile([1, R], F32, tag="rsr")
    V.reciprocal(rsr[:], rsum[:])
    s1c = wp.tile([1, R], F32, tag="s1c")
    V.tensor_tensor(s1c[:], rsr[:], modes1[0:1, :, t], op=OP.mult)
    s1cb_p = psS.tile([128, R], F32, tag="s")
    P.matmul(s1cb_p[:], ones[0:1, :], s1c[:])

    cnt = wp.tile([128, NT, R], F32, tag="cnt")
    V.tensor_tensor(cnt[:], rexp_pm[:].rearrange("p (c r) -> p c r", r=R),
                    s1cb_p[:, None, :].broadcast_to([128, NT, R]), op=OP.mult)
    rwT_n = sp.tile([128, NT * R], F32, tag="rwT")
    if t > 0:
        rwT_p = psS.tile([128, NT * R], F32, tag="s")
        for b in range(NT):
            blk = slice(128 * b, 128 * (b + 1))
            for c in range(NT):
                P.matmul(rwT_p[:, R * b:R * (b + 1)], L_n[:, c, blk],
                         rwTm0[:, R * c:R * (c + 1)],
                         start=(c == 0), stop=False)
            for c in range(NT):
                P.matmul(rwT_p[:, R * b:R * (b + 1)], LT_n[:, c, blk],
                         rwTm2[:, R * c:R * (c + 1)],
                         start=False, stop=(c == NT - 1))
        s02 = wp.tile([128, NT, R], F32, tag="s02")
        G_.tensor_tensor(s02[:], rwTm0[:].rearrange("p (c r) -> p c r", r=R),
                        rwTm2[:].rearrange("p (c r) -> p c r", r=R),
                        op=OP.add)
        corr = wp.tile([128, NT, R], F32, tag="corr")
        V.tensor_tensor(corr[:], s02[:],
                        dL_n[:, :, None].broadcast_to([128, NT, R]),
                        op=OP.mult)
        cnt2 = wp.tile([128, NT, R], F32, tag="cnt2")
        G_.tensor_tensor(cnt2[:], cnt[:], corr[:], op=OP.subtract)
        V.tensor_tensor(rwT_n[:], cnt2[:].rearrange("p c r -> p (c r)"),
                        rwT_p[:], op=OP.add)
    else:
        V.tensor_copy(rwT_n[:], cnt[:].rearrange("p c r -> p (c r)"))

    rwd_p = psS.tile([W, R], F32, tag="s")
    for c in range(NT):
        P.matmul(rwd_p[:], mem_nrm_n[:, c, :],
                 rwT_n[:, R * c:R * (c + 1)],
                 start=(c == 0), stop=(c == NT - 1))
    S.copy(out_sb[:, t, :], rwd_p[:])

    return dict(memT=memT_n, mem_nrm=mem_nrm_n, mnorm=mnorm_n, L=L_n,
                LT=LT_n, dL=dL_n, u_pm=u_pm_n, na_pm=na_pm_n,
                nsa=nsa_n, prec_pm=prec_pm_n, prec_fl=prec_fl_n, pbs=pbs,
                rwT=rwT_n)


# ---------------------------------------------------------------------------
_NC_CACHE = {}


def _get_nc():
    if "nc" not in _NC_CACHE:
        _NC_CACHE["nc"] = build_nc()
    return _NC_CACHE["nc"]


def _consts():
    ident = np.eye(128, dtype=np.float32)
    return (ident,)


def make_in_maps(controller_output, W_if, b_if, memory0):
    (ident,) = _consts()
    maps = []
    for b in range(B):
        maps.append({
            "co": np.ascontiguousarray(controller_output[b]),
            "wif": np.ascontiguousarray(W_if),
            "bif": np.ascontiguousarray(b_if.reshape(1, IF)),
            "mem0": np.ascontiguousarray(memory0[b]),
            "ident": ident,
        })
    return maps


def kernel(controller_output, W_if, b_if, memory0):
    from concourse.bass_utils import run_bass_kernel_spmd
    controller_output = np.asarray(controller_output, dtype=np.float32)
    W_if = np.asarray(W_if, dtype=np.float32)
    b_if = np.asarray(b_if, dtype=np.float32)
    memory0 = np.asarray(memory0, dtype=np.float32)
    nc = _get_nc()
    maps = make_in_maps(controller_output, W_if, b_if, memory0)
    # Retry once on non-finite output: a stale device (e.g. after an
    # earlier aborted run) can poison cores on the first dispatch.
    for _ in range(2):
        res = run_bass_kernel_spmd(nc, maps, core_ids=list(range(B)))
        out = np.stack([res.results[b]["out"] for b in range(B)], axis=0)
        if np.isfinite(out).all():
            break
    return out


if __name__ == "__main__":
    mode = sys.argv[1] if len(sys.argv) > 1 else "sim"
    sys.path.insert(0, "/root/problem")
    import jax
    with jax.default_device(jax.devices("cpu")[0]):
        import reference
        inputs = {k: np.asarray(v) for k, v in reference.setup_inputs().items()}
        expected = np.asarray(reference.reference(**inputs))

    if mode == "sim":
        from concourse.bass_interp import CoreSim
        nc = build_nc()
        maps = make_in_maps(inputs["controller_output"], inputs["W_if"],
                            inputs["b_if"], inputs["memory0"])
        sim = CoreSim(nc)
        for k, v in maps[0].items():
            sim.tensor(k)[:] = v
        sim.simulate()
        got = sim.tensor("out").copy()
        exp = expected[0]
        err = np.abs(got - exp)
        rel = np.linalg.norm(got - exp) / (np.linalg.norm(exp) + 1e-12)
        print("sim modeled time (ns):", sim.time)
        print("max abs err:", err.max(), " rel err:", rel)
    else:
        got = kernel(**inputs)
        rel = np.linalg.norm(got - expected) / (np.linalg.norm(expected) + 1e-12)
        print("max abs err:", np.abs(got - expected).max(), " rel err:", rel)


# revision 12
# speedup vs baseline: 1.7379x; 1.0247x over previous
"""DNC MemoryAccess kernel for Trainium2 (Bass/Tile), data-parallel over batch.

Shapes (hardcoded): B=8, T=16, C=1024, IFACE=471, N=512, WORD=64, R=4, NW=1.
Each of the 8 cores processes one batch element; all recurrent state stays
SBUF-resident across the T=16 sequential steps.

Design (vs the fp32 predecessor, 326us -> 192us modeled):
- the temporal link matrix L and its transpose LT are held in bf16; the
  elementwise recurrence L' = (1-w_i-w_j)L + w_i p_j runs as fast-mode
  tensor_scalar ops (0.25x DVE cycles in bf16) for w1 = w_j-(1-w_i) and the
  rank-1 terms, with the tensor_tensor multiplies/combines split across
  Pool and DVE,
- the link diagonal is never zeroed in-place: the scalar recurrence
  d' = (1-2w)d + w p is tracked in [128,NT] and its contribution is
  subtracted from the fwd/bwd PE matmul results,
- broadcast matmuls (ww, prec over partitions/words) use bf16 operands
  (1 PE cycle/row vs 4 for fp32); the usage broadcast for the allocation
  sort compare stays exact fp32 so sort ties match the fp32 reference,
- ln(usage) for the allocation cumprod and the memory-norm rsqrt use the
  Activation-table Ln/Exp; get_activation_tables is patched (membership
  only, original set order preserved) so Exp and Ln resolve to the one
  act-func set that contains both, hoisting the 1.3us table load out of
  the step loop,
- emission order is tuned for the per-engine in-order queues: the read
  softmax, rwTm scaling and dL tracker are emitted so the DVE queue never
  head-blocks the ww chain of the next step; the precedence flat vector
  is produced by PE transposes + one Act copy,
- float32r matmuls are NOT used: they fail neuronxcc BIR verification in
  this toolchain (sim accepts them; hardware compile rejects).

Precision: bf16 rounds the link matrices and the write/erase broadcasts
(~1e-3 relative on the output); usage comparisons stay exact fp32 so the
allocation sort matches the reference except for genuine fp32 ties (b=7
carries one, same as the fp32 baseline).
"""
import sys

sys.path.insert(0, "/opt/trn_rl_repo")

import numpy as np

import concourse.bacc as bacc
import concourse.bass as bass
import concourse.mybir as mybir
import concourse.tile as tile

F32 = mybir.dt.float32
F32R = mybir.dt.float32r
BF16 = mybir.dt.bfloat16
I32 = mybir.dt.int32
AF = mybir.ActivationFunctionType
OP = mybir.AluOpType

B, T, C, IF = 8, 16, 1024, 471
N, W, R = 512, 64, 4
NT = N // 128

O_RK, O_RS, O_WK, O_WS = 0, 256, 260, 324
O_ER, O_WV, O_FG, O_AG, O_WG, O_MD = 325, 389, 453, 457, 458, 459


def fr(ap):
    return ap


# Prefer the activation-function set that contains Exp AND Ln (plus
# Copy/Square/Sign), so the per-step Exp/Ln mix resolves to one table and the
# compiler hoists a single LoadActFuncSet out of the step loop instead of
# thrashing 1283ns loads between exp-only and ln-only sets.
_ORIG_GET_ACT_TABLES = None


def _patch_act_tables():
    global _ORIG_GET_ACT_TABLES
    if _ORIG_GET_ACT_TABLES is not None:
        return
    import concourse.hw_specs as hw_specs
    _ORIG_GET_ACT_TABLES = hw_specs.get_activation_tables

    def pinned(arch):
        tabs = dict(_ORIG_GET_ACT_TABLES(arch))
        pref = "natural_log_exp_and_others"
        if pref not in tabs:
            return tabs
        exp_ln = {mybir.ActivationFunctionType.Exp,
                  mybir.ActivationFunctionType.Ln}
        out = {}
        for k, v in tabs.items():
            out[k] = set(v) if k == pref else set(v) - exp_ln
        return out

    bacc.get_activation_tables = pinned


def build_nc():
    _patch_act_tables()
    nc = bacc.Bacc("TRN2", target_bir_lowering=False, debug=False, num_devices=8)

    co_d = nc.declare_dram_parameter("co", [T, C], F32, isOutput=False)
    w_d = nc.declare_dram_parameter("wif", [C, IF], F32, isOutput=False)
    b_d = nc.declare_dram_parameter("bif", [1, IF], F32, isOutput=False)
    m0_d = nc.declare_dram_parameter("mem0", [N, W], F32, isOutput=False)
    ident_d = nc.declare_dram_parameter("ident", [128, 128], F32, isOutput=False)
    out_d = nc.declare_dram_parameter("out", [T, R, W], F32, isOutput=True)

    with tile.TileContext(nc) as tc:
        with (
            nc.allow_low_precision(reason="bf16 link + f32r broadcasts stay"
                                   " within the 2e-2 gate"),
            tc.tile_pool(name="const", bufs=1) as cp,
            tc.tile_pool(name="state", bufs=2) as sp,
            tc.tile_pool(name="work", bufs=2) as wp,
            tc.tile_pool(name="psBig", bufs=1, space="PSUM") as psB,
            tc.tile_pool(name="psMem", bufs=1, space="PSUM") as psM,
            tc.tile_pool(name="psS", bufs=2, space="PSUM") as psS,
        ):
            _build_body(nc, tc, cp, sp, wp, psB, psM, psS,
                        co_d, w_d, b_d, m0_d, ident_d, out_d)
    nc.compile()
    return nc


def _build_body(nc, tc, cp, sp, wp, psB, psM, psS,
                co_d, w_d, b_d, m0_d, ident_d, out_d):
    V, S, P, G_, DMA = nc.vector, nc.scalar, nc.tensor, nc.gpsimd, nc.sync

    # ---------------- constants ----------------
    ident = cp.tile([128, 128], F32)
    DMA.dma_start(ident[:], ident_d[:])
    ones = cp.tile([128, 128], F32)
    G_.memset(ones[:], 1.0)
    ones_b = cp.tile([1, 128], BF16)
    G_.memset(ones_b[:], 1.0)

    # persistent per-t tables
    iface = cp.tile([T, IF], F32)          # raw iface rows
    wvR = cp.tile([1, T, W], BF16)         # write vectors, partition-0 rows
    keysc = cp.tile([W, 5, T], F32)        # scaled keys: r=0..3 read, 4 write
    neg_er = cp.tile([W, T], F32)
    gr = cp.tile([1, 6, T], F32)           # sigmoids: fg x4, ag, wg
    c1p = cp.tile([1, T], F32)
    cn1 = cp.tile([1, T], F32)
    c2 = cp.tile([1, T], F32)
    modes1 = cp.tile([1, R, T], F32)       # content-mode row per t
    mbs0 = cp.tile([128, R, T], F32)
    mbs2 = cp.tile([128, R, T], F32)
    nege0_pm = cp.tile([128, NT], F32)
    G_.memset(nege0_pm[:], 0.0)
    G_.memset(nege0_pm[0:1, 0:1], -1.0)
    out_sb = cp.tile([W, T, R], F32)

    # ---------------- prologue ----------------
    with tc.tile_pool(name="prolog", bufs=1) as pp:
        co_sb = pp.tile([T, C], F32)
        DMA.dma_start(co_sb[:], co_d[:])
        bif_sb = pp.tile([1, IF], F32)
        DMA.dma_start(bif_sb[:], b_d[:])
        w_sb = pp.tile([128, 8, IF], F32)
        for k in range(8):
            # split the 1.9MB load across two hwdge queues
            eng = DMA if k % 2 == 0 else nc.scalar
            eng.dma_start(w_sb[:, k, :], w_d[128 * k:128 * (k + 1), :])

        coT_p = psB.tile([128, 8, T], F32, tag="wb")
        for k in range(8):
            P.transpose(coT_p[:, k, :], co_sb[:, 128 * k:128 * (k + 1)],
                        ident[0:T, 0:T])
        coT = pp.tile([128, 8, T], F32)
        V.tensor_copy(coT[:], coT_p[:])

        if_p = psB.tile([T, IF], F32, tag="pb", bufs=2)
        for k in range(8):
            P.matmul(if_p[:], coT[:, k, :], w_sb[:, k, :],
                     start=(k == 0), stop=False)
        P.matmul(if_p[:], ones[0:1, 0:T], bif_sb[:],
                 start=False, stop=True)
        V.tensor_copy(iface[:], if_p[:])

        # keys [64, 5, T]: read r=0..3, write at 4
        keys_p = psB.tile([W, 5, T], F32, tag="pb", bufs=2)
        for r in range(R):
            P.transpose(keys_p[:, r, :], iface[:, O_RK + W * r:O_RK + W * (r + 1)],
                        ident[0:T, 0:T])
        P.transpose(keys_p[:, 4, :], iface[:, O_WK:O_WK + W], ident[0:T, 0:T])
        keys = pp.tile([W, 5, T], F32)
        V.tensor_copy(keys[:], keys_p[:])

        # write vectors as partition-0 rows via selector matmuls, two copies
        for h in range(2):
            wv_p = psB.tile([1, 8, W], F32, tag="pb", bufs=2, name=f"wvp{h}")
            for j in range(8):
                tt_ = 8 * h + j
                P.matmul(wv_p[0:1, j, :], ident[0:T, tt_:tt_ + 1],
                         iface[:, O_WV:O_WV + W])
            V.tensor_copy(wvR[0:1, 8 * h:8 * (h + 1), :].rearrange(
                "o t w -> o (t w)"),
                wv_p[:].rearrange("o t w -> o (t w)"))

        # erase sigmoid -> neg_er
        er_p = psS.tile([W, T], F32, tag="s")
        P.transpose(er_p[:], iface[:, O_ER:O_ER + W], ident[0:T, 0:T])
        ee = pp.tile([W, T], F32)
        S.activation(ee[:], er_p[:], AF.Exp, scale=-1.0)
        ew = pp.tile([W, T], F32)
        V.tensor_scalar(ew[:], ee[:], 1.0, None, op0=OP.add)
        er_r = pp.tile([W, T], F32)
        V.reciprocal(er_r[:], ew[:])
        V.tensor_scalar(neg_er[:], er_r[:], -1.0, None, op0=OP.mult)

        # strengths softplus: [1, 5, T] (rs x4, ws)
        sts_p = psS.tile([1, 5, T], F32, tag="s")
        for r in range(R):
            P.transpose(sts_p[0:1, r, :], iface[:, O_RS + r:O_RS + r + 1],
                        ident[0:T, 0:T])
        P.transpose(sts_p[0:1, 4, :], iface[:, O_WS:O_WS + 1], ident[0:T, 0:T])
        st_e = pp.tile([1, 5 * T], F32)
        S.activation(st_e[:], sts_p[:].rearrange("o f t -> o (f t)"), AF.Exp)
        st_w = pp.tile([1, 5 * T], F32)
        V.tensor_scalar(st_w[:], st_e[:], 1.0, None, op0=OP.add)
        st_sp = pp.tile([1, 5 * T], F32)
        S.activation(st_sp[:], st_w[:], AF.Ln)

        # key norms: rsqrt(sum keys^2) = exp(-0.5 ln)
        sqk = pp.tile([W, 5 * T], F32)
        S.activation(sqk[:], keys[:].rearrange("w f t -> w (f t)"), AF.Square)
        k2_p = psM.tile([1, 5 * T], F32, tag="wwb")
        P.matmul(k2_p[:], ones[0:W, 0:1], sqk[:])
        lk2 = pp.tile([1, 5 * T], F32)
        S.activation(lk2[:], k2_p[:], AF.Ln)
        kr = pp.tile([1, 5 * T], F32)
        S.activation(kr[:], lk2[:], AF.Exp, scale=-0.5)
        beta = pp.tile([1, 5 * T], F32)
        V.tensor_tensor(beta[:], st_sp[:], kr[:], op=OP.mult)
        kb_p = psM.tile([W, 5 * T], F32, tag="add")
        P.matmul(kb_p[:], ones[0:1, 0:W], beta[:])
        V.tensor_tensor(keysc[:].rearrange("w f t -> w (f t)"),
                        keys[:].rearrange("w f t -> w (f t)"), kb_p[:],
                        op=OP.mult)

        # gates: fg x4, ag, wg sigmoids
        gats_p = psS.tile([1, 6, T], F32, tag="s")
        for r in range(R):
            P.transpose(gats_p[0:1, r, :], iface[:, O_FG + r:O_FG + r + 1],
                        ident[0:T, 0:T])
        P.transpose(gats_p[0:1, 4, :], iface[:, O_AG:O_AG + 1], ident[0:T, 0:T])
        P.transpose(gats_p[0:1, 5, :], iface[:, O_WG:O_WG + 1], ident[0:T, 0:T])
        g_e = pp.tile([1, 6 * T], F32)
        S.activation(g_e[:], gats_p[:].rearrange("o g t -> o (g t)"), AF.Exp,
                     scale=-1.0)
        g_w = pp.tile([1, 6 * T], F32)
        V.tensor_scalar(g_w[:], g_e[:], 1.0, None, op0=OP.add)
        V.reciprocal(gr[:].rearrange("o g t -> o (g t)"), g_w[:])
        ag_t = gr[0:1, 4, :]
        wg_t = gr[0:1, 5, :]
        V.tensor_tensor(c1p[:], ag_t, wg_t, op=OP.mult)
        V.tensor_scalar(cn1[:], c1p[:], -1.0, None, op0=OP.mult)
        V.tensor_tensor(c2[:], wg_t, c1p[:], op=OP.subtract)

        # modes softmax -> rows per t
        me = pp.tile([T, 12], F32)
        S.activation(me[:], iface[:, O_MD:O_MD + 12], AF.Exp)
        me3 = me[:].rearrange("t (r m) -> t r m", m=3)
        msum = pp.tile([T, R], F32)
        V.tensor_tensor(msum[:], me3[:, :, 0], me3[:, :, 1], op=OP.add)
        V.tensor_tensor(msum[:], msum[:], me3[:, :, 2], op=OP.add)
        mrcp = pp.tile([T, R], F32)
        V.reciprocal(mrcp[:], msum[:])
        mn = pp.tile([T, 12], F32)
        mn3 = mn[:].rearrange("t (m r) -> t m r", r=R)
        me3b = me[:].rearrange("t (r m) -> t m r", m=3)
        for m in range(3):
            V.tensor_tensor(mn3[:, m, :], me3b[:, m, :], mrcp[:], op=OP.mult)
        # three m-blocks at base partition 0: modes0/1/2 [4, T]
        mblk_p = psS.tile([R, 3, T], F32, tag="s")
        for m in range(3):
            P.transpose(mblk_p[:, m, :], mn[:, 4 * m:4 * (m + 1)],
                        ident[0:T, 0:T])
        mblk = pp.tile([R, 3, T], F32)
        V.tensor_copy(mblk[:], mblk_p[:])
        m1sel_p = psS.tile([1, R, T], F32, tag="s")
        for r in range(R):
            P.matmul(m1sel_p[0:1, r, :], ident[0:R, r:r + 1], mblk[:, 1, :])
        V.tensor_copy(modes1[:].rearrange("o r t -> o (r t)"),
                      m1sel_p[:].rearrange("o r t -> o (r t)"))
        # flatten rows r of m-block 0/2 onto partition 0 via selector matmuls
        mrows_p = psS.tile([1, 2, R, T], F32, tag="s")
        for r in range(R):
            P.matmul(mrows_p[0:1, 0, r, :], ident[0:R, r:r + 1], mblk[:, 0, :])
            P.matmul(mrows_p[0:1, 1, r, :], ident[0:R, r:r + 1], mblk[:, 2, :])
        mrows = pp.tile([1, 2, R, T], F32)
        V.tensor_copy(mrows[:].rearrange("o a r t -> o (a r t)"),
                      mrows_p[:].rearrange("o a r t -> o (a r t)"))
        mb0_p = psB.tile([128, R * T], F32, tag="wb")
        P.matmul(mb0_p[:], ones[0:1, :], mrows[0:1, 0, :, :])
        V.tensor_copy(mbs0[:].rearrange("p r t -> p (r t)"), mb0_p[:])
        mb2_p = psB.tile([128, R * T], F32, tag="pb", bufs=2)
        P.matmul(mb2_p[:], ones[0:1, :], mrows[0:1, 1, :, :])
        V.tensor_copy(mbs2[:].rearrange("p r t -> p (r t)"), mb2_p[:])

    # ---------------- initial state ----------------
    mem_nrm = sp.tile([128, NT, W], F32, tag="mem_nrm")
    for c in range(NT):
        DMA.dma_start(mem_nrm[:, c, :],
                      m0_d[128 * c:128 * (c + 1), :])
    memT_p = psB.tile([W, N], F32, tag="wb")
    for c in range(NT):
        P.transpose(memT_p[:, 128 * c:128 * (c + 1)],
                    mem_nrm[:, c, :], ident[:])
    memT = sp.tile([W, N], F32, tag="memT")
    V.tensor_copy(memT[:], memT_p[:])

    # initial norm: PM-layout sqn -> msq -> Ln/Exp
    sqn0 = wp.tile([128, NT, W], F32, tag="sqn")
    G_.tensor_tensor(sqn0[:], mem_nrm[:], mem_nrm[:], op=OP.mult)
    msq0 = wp.tile([128, NT], F32, tag="msq")
    V.tensor_reduce(msq0[:], sqn0[:], axis=mybir.AxisListType.X, op=OP.add)
    lms0 = wp.tile([128, NT], F32, tag="lms")
    S.activation(lms0[:], msq0[:], AF.Ln)
    mnorm_i = sp.tile([128, NT], F32, tag="mnorm")
    S.activation(mnorm_i[:], lms0[:], AF.Exp, scale=-0.5)

    L = sp.tile([128, NT, N], BF16, tag="L")
    G_.memset(L[:], 0.0)
    LT0 = sp.tile([128, NT, N], BF16, tag="LT")
    G_.memset(LT0[:], 0.0)
    dL0 = sp.tile([128, NT], F32, tag="dL")
    G_.memset(dL0[:], 0.0)

    st = dict(memT=memT, mem_nrm=mem_nrm, mnorm=mnorm_i, L=L, LT=LT0,
              dL=dL0, u_pm=None, prec_pm=None, prec_fl=None,
              pbs=None, rwT=None)

    for t in range(T):
        st = _step(nc, t, st, cp, sp, wp, psB, psM, psS,
                   ident, ones, ones_b, iface, wvR, keysc, neg_er, gr, c1p,
                   cn1, c2, modes1, mbs0, mbs2, nege0_pm, out_sb)

    DMA.dma_start(out_d[:].rearrange("t r w -> w t r"), out_sb[:])


def _step(nc, t, st, cp, sp, wp, psB, psM, psS,
          ident, ones, ones_b, iface, wvR, keysc, neg_er, gr, c1p, cn1, c2,
          modes1, mbs0, mbs2, nege0_pm, out_sb):
    V, S, P, G_, DMA = nc.vector, nc.scalar, nc.tensor, nc.gpsimd, nc.sync
    memT, mem_nrm, mnorm = st["memT"], st["mem_nrm"], st["mnorm"]
    L, LT, dL, u_pm = st["L"], st["LT"], st["dL"], st["u_pm"]
    prec_pm, prec_fl, pbs, rwT = (st["prec_pm"], st["prec_fl"], st["pbs"],
                                  st["rwT"])
    last = (t == T - 1)

    if t == 0:
        na_pm, nsa = nege0_pm, None
    else:
        na_pm, nsa = st["na_pm"], st["nsa"]

    # ---- pbs broadcast (prev-step prec; runs at step start) ----
    if t > 0:
        pb_p = psB.tile([128, N], F32, tag="pb", bufs=2)
        P.matmul(pb_p[:], ones_b[0:1, :], prec_fl[:])
        pbs = wp.tile([128, N], BF16, tag="pbs")
        S.activation(pbs[:], pb_p[:], AF.Copy)

    # ---- write content softmax (PM) ----
    wdots_p = psS.tile([128, NT], F32, tag="s")
    for b in range(NT):
        P.matmul(wdots_p[:, b:b + 1], memT[:, 128 * b:128 * (b + 1)],
                 keysc[:, 4, t:t + 1])
    wlog = wp.tile([128, NT], F32, tag="wlog")
    V.tensor_tensor(wlog[:], wdots_p[:], mnorm[:], op=OP.mult)
    wexp_pm = wp.tile([128, NT], F32, tag="wexp")
    S.activation(wexp_pm[:], wlog[:], AF.Exp)
    wps_p = psS.tile([1, NT], F32, tag="s")
    P.matmul(wps_p[:], ones[:, 0:1], wexp_pm[:])
    wsum = wp.tile([1, 1], F32, tag="wsum")
    V.tensor_reduce(wsum[:], wps_p[:], axis=mybir.AxisListType.X, op=OP.add)
    wrs = wp.tile([1, 1], F32, tag="wrs")
    V.reciprocal(wrs[:], wsum[:])
    cw = wp.tile([1, 1], F32, tag="cw")
    V.tensor_tensor(cw[:], wrs[:], c2[0:1, t:t + 1], op=OP.mult)

    # ---- ww assembly (PM) ----
    cn1b_p = psS.tile([128, 1], F32, tag="s")
    P.matmul(cn1b_p[:], ones[0:1, :], cn1[0:1, t:t + 1])
    cwb_p = psS.tile([128, 1], F32, tag="s")
    P.matmul(cwb_p[:], ones[0:1, :], cw[:])
    wwx = wp.tile([128, NT], F32, tag="wwx")
    V.tensor_scalar(wwx[:], na_pm[:], cn1b_p[:, 0:1], None, op0=OP.mult)
    ww_pm = wp.tile([128, NT], F32, tag="wwpm")
    V.scalar_tensor_tensor(ww_pm[:], wexp_pm[:], cwb_p[:, 0:1], wwx[:],
                           op0=OP.mult, op1=OP.add)
    if t > 0:
        omw_pm = wp.tile([128, NT], F32, tag="omw")
        V.tensor_scalar(omw_pm[:], ww_pm[:], -1.0, 1.0, op0=OP.mult,
                        op1=OP.add)
    if st.get("fin") is not None:
        st["fin"]()
        st["fin"] = None

    ww_tp = psS.tile([1, N], F32, tag="s")
    for c in range(NT):
        P.transpose(ww_tp[0:1, 128 * c:128 * (c + 1)], ww_pm[:, c:c + 1],
                    ident[:])
    ww_fl = wp.tile([1, N], BF16, tag="wwfl")
    S.copy(ww_fl[:], ww_tp[:])

    # ---- wbs broadcast (bf16, for the link ts ops) ----
    if t > 0:
        wb_p = psB.tile([128, N], F32, tag="wb")
        P.matmul(wb_p[:], ones_b[0:1, :], ww_fl[:])
        wbs = wp.tile([128, N], BF16, tag="wbs")
        S.activation(wbs[:], wb_p[:], AF.Copy)

    # ---- memory head ----
    wwb_p = psM.tile([W, N], F32, tag="wwb")
    P.matmul(wwb_p[:], ones_b[0:1, 0:W], ww_fl[:])
    add_p = psM.tile([W, N], F32, tag="add")
    P.matmul(add_p[:], wvR[0:1, t, :], ww_fl[:])
    keep = wp.tile([W, N], F32, tag="keep")
    S.activation(keep[:], wwb_p[:], AF.Copy, scale=neg_er[:, t:t + 1],
                 bias=1.0)
    m1 = wp.tile([W, N], F32, tag="m1")
    G_.tensor_tensor(m1[:], memT[:], keep[:], op=OP.mult)

    # ---- usage update ----
    if last:
        u_pm_n = u_pm
    else:
        u_pm_n = sp.tile([128, NT], F32, tag="u_pm")
        if t == 0:
            V.tensor_copy(u_pm_n[:], ww_pm[:])
        else:
            fgb_p = psS.tile([128, R], F32, tag="s")
            P.matmul(fgb_p[:], ones[0:1, :], gr[0:1, 0:R, t])
            yyT = wp.tile([128, NT, R], F32, tag="yyT")
            V.scalar_tensor_tensor(
                yyT[:], fgb_p[:, None, :].broadcast_to([128, NT, R]), -1.0,
                rwT[:].rearrange("p (c r) -> p c r", r=R),
                op0=OP.mult, op1=OP.mult)
            om = wp.tile([128, NT, R], F32, tag="om")
            V.tensor_scalar(om[:], yyT[:], 1.0, None, op0=OP.add)
            p1u = wp.tile([128, NT], F32, tag="p1u")
            G_.tensor_tensor(p1u[:], om[:, :, 0], om[:, :, 1], op=OP.mult)
            p2u = wp.tile([128, NT], F32, tag="p2u")
            G_.tensor_tensor(p2u[:], om[:, :, 2], om[:, :, 3], op=OP.mult)
            psi = wp.tile([128, NT], F32, tag="psi")
            G_.tensor_tensor(psi[:], p1u[:], p2u[:], op=OP.mult)
            omu = wp.tile([128, NT], F32, tag="omu")
            V.tensor_scalar(omu[:], u_pm[:], -1.0, 1.0, op0=OP.mult,
                            op1=OP.add)
            tn = wp.tile([128, NT], F32, tag="tn")
            V.scalar_tensor_tensor(tn[:], ww_pm[:], 1.0, omu[:],
                                   op0=OP.subtract, op1=OP.mult)
            V.scalar_tensor_tensor(u_pm_n[:], tn[:], 1.0, psi[:],
                                   op0=OP.add, op1=OP.mult)

    # ---- allocation compare inputs (flat u + broadcast; exact fp32) ----
    if not last:
        u_tp = psS.tile([1, N], F32, tag="s")
        for c in range(NT):
            P.transpose(u_tp[0:1, 128 * c:128 * (c + 1)], u_pm_n[:, c:c + 1],
                        ident[:])
        u_fl_n = wp.tile([1, N], F32, tag="ufl")
        V.tensor_copy(u_fl_n[:], u_tp[:])
        ub_p = psM.tile([128, N], F32, tag="wwb")
        P.matmul(ub_p[:], ones[0:1, :], u_fl_n[:])
        ubs = wp.tile([128, N], F32, tag="ubs")
        S.copy(ubs[:], ub_p[:])
        ucl = wp.tile([128, NT], F32, tag="ucl")
        V.tensor_scalar(ucl[:], u_pm_n[:], 1e-38, None, op0=OP.max)
        lnu = wp.tile([128, NT], F32, tag="lnu")
        S.activation(lnu[:], ucl[:], AF.Ln)

    # ---- prec update ----
    if not last:
        prec_pm_n = sp.tile([128, NT], F32, tag="prec_pm")
        if t == 0:
            V.tensor_copy(prec_pm_n[:], ww_pm[:])
        else:
            swa = wp.tile([1, 1], F32, tag="swa")
            G_.tensor_tensor(swa[:], nsa[:], cn1[0:1, t:t + 1], op=OP.mult)
            sw = wp.tile([1, 1], F32, tag="sw")
            G_.tensor_tensor(sw[:], swa[:], c2[0:1, t:t + 1], op=OP.add)
            omsw = wp.tile([1, 1], F32, tag="omsw")
            V.tensor_scalar(omsw[:], sw[:], -1.0, 1.0, op0=OP.mult,
                            op1=OP.add)
            omsw_p = psS.tile([128, 1], F32, tag="s")
            P.matmul(omsw_p[:], ones[0:1, :], omsw[:])
            V.scalar_tensor_tensor(prec_pm_n[:], prec_pm[:], omsw_p[:, 0:1],
                                   ww_pm[:], op0=OP.mult, op1=OP.add)
        p_tp = psS.tile([1, N], F32, tag="s")
        for c in range(NT):
            P.transpose(p_tp[0:1, 128 * c:128 * (c + 1)], prec_pm_n[:, c:c + 1],
                        ident[:])
        prec_fl_n = sp.tile([1, N], BF16, tag="prec_fl")
        S.copy(prec_fl_n[:], p_tp[:])
    else:
        prec_pm_n, prec_fl_n = prec_pm, prec_fl

    # ---- mode-scaled read weights + link diagonal tracker ----

    # ---- link loop with interleaved memT_n / Gt compares ----
    comb_eng = [(G_, G_), (G_, V), (G_, V), (G_, V)]
    if t == 0:
        L_n, LT_n = L, LT
        memT_n = sp.tile([W, N], F32, tag="memT")
        V.tensor_tensor(memT_n[:], m1[:], add_p[:], op=OP.add)
        if not last:
            Gt_n = wp.tile([128, NT, N], F32, tag="G", bufs=1)
            for c in range(NT):
                V.tensor_scalar(Gt_n[:, c, :], ubs[:], u_pm_n[:, c:c + 1],
                                None, op0=OP.is_gt)
    else:
        L_n = sp.tile([128, NT, N], BF16, tag="L")
        LT_n = sp.tile([128, NT, N], BF16, tag="LT")
        memT_n = sp.tile([W, N], F32, tag="memT")
        if not last:
            Gt_n = wp.tile([128, NT, N], F32, tag="G", bufs=1)
        for c in range(NT):
            w1 = wp.tile([128, N], BF16, tag=f"w1_{c % 2}")
            V.tensor_scalar(w1[:], wbs[:], omw_pm[:, c:c + 1], None,
                            op0=OP.subtract)
            p1 = wp.tile([128, N], BF16, tag=f"p1_{c % 2}")
            V.tensor_scalar(p1[:], pbs[:], ww_pm[:, c:c + 1], None,
                            op0=OP.mult)
            p1T = wp.tile([128, N], BF16, tag=f"p1T_{c % 2}")
            V.tensor_scalar(p1T[:], wbs[:], prec_pm[:, c:c + 1], None,
                            op0=OP.mult)
            t1 = wp.tile([128, N], BF16, tag=f"t1_{c % 2}")
            G_.tensor_tensor(t1[:], w1[:], L[:, c, :], op=OP.mult)
            t1T = wp.tile([128, N], BF16, tag=f"t1T_{c % 2}")
            G_.tensor_tensor(t1T[:], w1[:], LT[:, c, :], op=OP.mult)
            eL, eLT = comb_eng[c]
            eL.tensor_tensor(L_n[:, c, :], p1[:], t1[:], op=OP.subtract)
            eLT.tensor_tensor(LT_n[:, c, :], p1T[:], t1T[:], op=OP.subtract)
            if c == 0:
                V.tensor_tensor(memT_n[:], m1[:], add_p[:], op=OP.add)
            elif not last:
                cc = c - 1
                V.tensor_scalar(Gt_n[:, cc, :], ubs[:],
                                u_pm_n[:, cc:cc + 1], None, op0=OP.is_gt)
        if not last:
            V.tensor_scalar(Gt_n[:, 3, :], ubs[:], u_pm_n[:, 3:4],
                            None, op0=OP.is_gt)

    if t > 0:
        wp_pm = wp.tile([128, NT], F32, tag="wppm")
        G_.tensor_tensor(wp_pm[:], ww_pm[:], prec_pm[:], op=OP.mult)
        dmul = wp.tile([128, NT], F32, tag="dmul")
        V.tensor_scalar(dmul[:], ww_pm[:], -2.0, 1.0, op0=OP.mult, op1=OP.add)
        dL_n = sp.tile([128, NT], F32, tag="dL")
        V.scalar_tensor_tensor(dL_n[:], dL[:], 1.0, dmul[:],
                               op0=OP.mult, op1=OP.mult)
        G_.tensor_tensor(dL_n[:], dL_n[:], wp_pm[:], op=OP.add)
    else:
        dL_n = dL
    if t > 0:
        rwTm0 = wp.tile([128, NT * R], BF16, tag="rwTm0")
        V.tensor_tensor(rwTm0[:].rearrange("p (c r) -> p c r", r=R),
                        rwT[:].rearrange("p (c r) -> p c r", r=R),
                        mbs0[:, None, :, t].broadcast_to([128, NT, R]),
                        op=OP.mult)
        rwTm2 = wp.tile([128, NT * R], BF16, tag="rwTm2")
        V.tensor_tensor(rwTm2[:].rearrange("p (c r) -> p c r", r=R),
                        rwT[:].rearrange("p (c r) -> p c r", r=R),
                        mbs2[:, None, :, t].broadcast_to([128, NT, R]),
                        op=OP.mult)

    # ---- memory norm chain ----
    mem_nrm_p = psS.tile([128, NT, W], F32, tag="mn", bufs=1)
    for c in range(NT):
        P.transpose(mem_nrm_p[:, c, :], memT_n[:, 128 * c:128 * (c + 1)],
                    ident[0:W, 0:W])
    sqn = wp.tile([128, NT, W], F32, tag="sqn")
    S.activation(sqn[:], mem_nrm_p[:], AF.Square)
    msq = wp.tile([128, NT], F32, tag="msq")
    V.tensor_reduce(msq[:], sqn[:], axis=mybir.AxisListType.X, op=OP.add)
    mem_nrm_n = sp.tile([128, NT, W], F32, tag="mem_nrm")
    S.copy(mem_nrm_n[:], mem_nrm_p[:])
    lms = wp.tile([128, NT], F32, tag="lms")
    S.activation(lms[:], msq[:], AF.Ln)
    mnorm_n = sp.tile([128, NT], F32, tag="mnorm")
    S.activation(mnorm_n[:], lms[:], AF.Exp, scale=-0.5)

    # ---- allocation log-sum ----
    if last:
        na_pm_n, nsa_n = None, None
    else:
        sT_p = psS.tile([128, NT], F32, tag="s")
        for b in range(NT):
            for c in range(NT):
                P.matmul(sT_p[:, b:b + 1], Gt_n[:, c, 128 * b:128 * (b + 1)],
                         lnu[:, c:c + 1], start=(c == 0),
                         stop=(c == NT - 1))
        es_pm = wp.tile([128, NT], F32, tag="espm")
        S.activation(es_pm[:], sT_p[:], AF.Exp)
        na_pm_n = wp.tile([128, NT], F32, tag="napm")
        if t < T - 2:
            nap = wp.tile([128, 1], F32, tag="nap")
            V.scalar_tensor_tensor(na_pm_n[:], u_pm_n[:], 1.0, es_pm[:],
                                   op0=OP.subtract, op1=OP.mult,
                                   accum_out=nap[:])
        else:
            V.scalar_tensor_tensor(na_pm_n[:], u_pm_n[:], 1.0, es_pm[:],
                                   op0=OP.subtract, op1=OP.mult)
        if t < T - 2:
            nsa_p = psS.tile([1, 1], F32, tag="s")
            P.matmul(nsa_p[:], nap[:], ones[:, 0:1])
            nsa_n = wp.tile([1, 1], F32, tag="nsa")
            V.tensor_copy(nsa_n[:], nsa_p[:])
        else:
            nsa_n = None

    # ---- read content (PM) ----
    rdots_p = psS.tile([128, NT * R], F32, tag="s")
    for b in range(NT):
        P.matmul(rdots_p[:, R * b:R * (b + 1)],
                 memT_n[:, 128 * b:128 * (b + 1)], keysc[:, 0:4, t])
    rlog = wp.tile([128, NT, R], F32, tag="rlog")
    V.tensor_tensor(rlog[:],
                    rdots_p[:].rearrange("p (c r) -> p c r", r=R),
                    mnorm_n[:, :, None].broadcast_to([128, NT, R]),
                    op=OP.mult)
    rexp_pm = wp.tile([128, NT * R], F32, tag="rexp")
    S.activation(rexp_pm[:], rlog[:].rearrange("p c r -> p (c r)"), AF.Exp)
    rps_p = psS.tile([1, NT * R], F32, tag="s")
    P.matmul(rps_p[:], ones[:, 0:1], rexp_pm[:])
    rsum = wp.tile([1, R], F32, tag="rsum")
    V.tensor_reduce(rsum[:], rps_p[:].rearrange("o (c r) -> o r c", r=R),
                    axis=mybir.AxisListType.X, op=OP.add)
    rsr = wp.tile([1, R], F32, tag="rsr")
    V.reciprocal(rsr[:], rsum[:])
    s1c = wp.tile([1, R], F32, tag="s1c")
    V.tensor_tensor(s1c[:], rsr[:], modes1[0:1, :, t], op=OP.mult)
    s1cb_p = psS.tile([128, R], F32, tag="s")
    P.matmul(s1cb_p[:], ones[0:1, :], s1c[:])

    cnt = wp.tile([128, NT, R], F32, tag="cnt")
    V.tensor_tensor(cnt[:], rexp_pm[:].rearrange("p (c r) -> p c r", r=R),
                    s1cb_p[:, None, :].broadcast_to([128, NT, R]), op=OP.mult)
    rwT_n = sp.tile([128, NT * R], F32, tag="rwT")
    if t > 0:
        rwT_p = psS.tile([128, NT * R], F32, tag="s")
        for b in range(NT):
            blk = slice(128 * b, 128 * (b + 1))
            for c in range(NT):
                P.matmul(rwT_p[:, R * b:R * (b + 1)], L_n[:, c, blk],
                         rwTm0[:, R * c:R * (c + 1)],
                         start=(c == 0), stop=False)
            for c in range(NT):
                P.matmul(rwT_p[:, R * b:R * (b + 1)], LT_n[:, c, blk],
                         rwTm2[:, R * c:R * (c + 1)],
                         start=False, stop=(c == NT - 1))
        s02 = wp.tile([128, NT, R], F32, tag="s02")
        G_.tensor_tensor(s02[:], rwTm0[:].rearrange("p (c r) -> p c r", r=R),
                        rwTm2[:].rearrange("p (c r) -> p c r", r=R),
                        op=OP.add)
        corr = wp.tile([128, NT, R], F32, tag="corr")
        V.tensor_tensor(corr[:], s02[:],
                        dL_n[:, :, None].broadcast_to([128, NT, R]),
                        op=OP.mult)
        cnt2 = wp.tile([128, NT, R], F32, tag="cnt2")
        G_.tensor_tensor(cnt2[:], cnt[:], corr[:], op=OP.subtract)
        def _fin(rwT_n=rwT_n, cnt2=cnt2, rwT_p=rwT_p, mem_nrm_n=mem_nrm_n,
                 t=t):
            V.tensor_tensor(rwT_n[:], cnt2[:].rearrange("p c r -> p (c r)"),
                            rwT_p[:], op=OP.add)
            rwd_p = psS.tile([W, R], F32, tag="s")
            for c in range(NT):
                P.matmul(rwd_p[:], mem_nrm_n[:, c, :],
                         rwT_n[:, R * c:R * (c + 1)],
                         start=(c == 0), stop=(c == NT - 1))
            S.copy(out_sb[:, t, :], rwd_p[:])
    else:
        V.tensor_copy(rwT_n[:], cnt[:].rearrange("p c r -> p (c r)"))
        def _fin(rwT_n=rwT_n, mem_nrm_n=mem_nrm_n, t=t):
            rwd_p = psS.tile([W, R], F32, tag="s")
            for c in range(NT):
                P.matmul(rwd_p[:], mem_nrm_n[:, c, :],
                         rwT_n[:, R * c:R * (c + 1)],
                         start=(c == 0), stop=(c == NT - 1))
            S.copy(out_sb[:, t, :], rwd_p[:])
    if last:
        _fin()
    else:
        st_fin = _fin

    return dict(memT=memT_n, mem_nrm=mem_nrm_n, mnorm=mnorm_n, L=L_n,
                LT=LT_n, dL=dL_n, u_pm=u_pm_n, na_pm=na_pm_n,
                nsa=nsa_n, prec_pm=prec_pm_n, prec_fl=prec_fl_n, pbs=pbs,
                rwT=rwT_n, fin=(None if last else st_fin))


# ---------------------------------------------------------------------------
_NC_CACHE = {}


def _get_nc():
    if "nc" not in _NC_CACHE:
        _NC_CACHE["nc"] = build_nc()
    return _NC_CACHE["nc"]


def _consts():
    ident = np.eye(128, dtype=np.float32)
    return (ident,)


def make_in_maps(controller_output, W_if, b_if, memory0):
    (ident,) = _consts()
    maps = []
    for b in range(B):
        maps.append({
            "co": np.ascontiguousarray(controller_output[b]),
            "wif": np.ascontiguousarray(W_if),
            "bif": np.ascontiguousarray(b_if.reshape(1, IF)),
            "mem0": np.ascontiguousarray(memory0[b]),
            "ident": ident,
        })
    return maps


def kernel(controller_output, W_if, b_if, memory0):
    from concourse.bass_utils import run_bass_kernel_spmd
    controller_output = np.asarray(controller_output, dtype=np.float32)
    W_if = np.asarray(W_if, dtype=np.float32)
    b_if = np.asarray(b_if, dtype=np.float32)
    memory0 = np.asarray(memory0, dtype=np.float32)
    nc = _get_nc()
    maps = make_in_maps(controller_output, W_if, b_if, memory0)
    # Retry once on non-finite output: a stale device (e.g. after an
    # earlier aborted run) can poison cores on the first dispatch.
    for _ in range(2):
        res = run_bass_kernel_spmd(nc, maps, core_ids=list(range(B)))
        out = np.stack([res.results[b]["out"] for b in range(B)], axis=0)
        if np.isfinite(out).all():
            break
    return out


if __name__ == "__main__":
    mode = sys.argv[1] if len(sys.argv) > 1 else "sim"
    sys.path.insert(0, "/root/problem")
    import jax
    with jax.default_device(jax.devices("cpu")[0]):
        import reference
        inputs = {k: np.asarray(v) for k, v in reference.setup_inputs().items()}
        expected = np.asarray(reference.reference(**inputs))

    if mode == "sim":
        from concourse.bass_interp import CoreSim
        nc = build_nc()
        maps = make_in_maps(inputs["controller_output"], inputs["W_if"],
                            inputs["b_if"], inputs["memory0"])
        sim = CoreSim(nc)
        for k, v in maps[0].items():
            sim.tensor(k)[:] = v
        sim.simulate()
        got = sim.tensor("out").copy()
        exp = expected[0]
        err = np.abs(got - exp)
        rel = np.linalg.norm(got - exp) / (np.linalg.norm(exp) + 1e-12)
        print("sim modeled time (ns):", sim.time)
        print("max abs err:", err.max(), " rel err:", rel)
    else:
        got = kernel(**inputs)
        rel = np.linalg.norm(got - expected) / (np.linalg.norm(expected) + 1e-12)
        print("max abs err:", np.abs(got - expected).max(), " rel err:", rel)


# revision 13
# speedup vs baseline: 1.7421x; 1.0024x over previous
"""DNC MemoryAccess kernel for Trainium2 (Bass/Tile), data-parallel over batch.

Shapes (hardcoded): B=8, T=16, C=1024, IFACE=471, N=512, WORD=64, R=4, NW=1.
Each of the 8 cores processes one batch element; all recurrent state stays
SBUF-resident across the T=16 sequential steps.

Design (vs the fp32 predecessor, 326us -> 192us modeled):
- the temporal link matrix L and its transpose LT are held in bf16; the
  elementwise recurrence L' = (1-w_i-w_j)L + w_i p_j runs as fast-mode
  tensor_scalar ops (0.25x DVE cycles in bf16) for w1 = w_j-(1-w_i) and the
  rank-1 terms, with the tensor_tensor multiplies/combines split across
  Pool and DVE,
- the link diagonal is never zeroed in-place: the scalar recurrence
  d' = (1-2w)d + w p is tracked in [128,NT] and its contribution is
  subtracted from the fwd/bwd PE matmul results,
- broadcast matmuls (ww, prec over partitions/words) use bf16 operands
  (1 PE cycle/row vs 4 for fp32); the usage broadcast for the allocation
  sort compare stays exact fp32 so sort ties match the fp32 reference,
- ln(usage) for the allocation cumprod and the memory-norm rsqrt use the
  Activation-table Ln/Exp; get_activation_tables is patched (membership
  only, original set order preserved) so Exp and Ln resolve to the one
  act-func set that contains both, hoisting the 1.3us table load out of
  the step loop,
- emission order is tuned for the per-engine in-order queues: the read
  softmax, rwTm scaling and dL tracker are emitted so the DVE queue never
  head-blocks the ww chain of the next step; the precedence flat vector
  is produced by PE transposes + one Act copy,
- float32r matmuls are NOT used: they fail neuronxcc BIR verification in
  this toolchain (sim accepts them; hardware compile rejects).

Precision: bf16 rounds the link matrices and the write/erase broadcasts
(~1e-3 relative on the output); usage comparisons stay exact fp32 so the
allocation sort matches the reference except for genuine fp32 ties (b=7
carries one, same as the fp32 baseline).
"""
import sys

sys.path.insert(0, "/opt/trn_rl_repo")

import numpy as np

import concourse.bacc as bacc
import concourse.bass as bass
import concourse.mybir as mybir
import concourse.tile as tile

F32 = mybir.dt.float32
F32R = mybir.dt.float32r
BF16 = mybir.dt.bfloat16
I32 = mybir.dt.int32
AF = mybir.ActivationFunctionType
OP = mybir.AluOpType

B, T, C, IF = 8, 16, 1024, 471
N, W, R = 512, 64, 4
NT = N // 128

O_RK, O_RS, O_WK, O_WS = 0, 256, 260, 324
O_ER, O_WV, O_FG, O_AG, O_WG, O_MD = 325, 389, 453, 457, 458, 459


def fr(ap):
    return ap


# Prefer the activation-function set that contains Exp AND Ln (plus
# Copy/Square/Sign), so the per-step Exp/Ln mix resolves to one table and the
# compiler hoists a single LoadActFuncSet out of the step loop instead of
# thrashing 1283ns loads between exp-only and ln-only sets.
_ORIG_GET_ACT_TABLES = None


def _patch_act_tables():
    global _ORIG_GET_ACT_TABLES
    if _ORIG_GET_ACT_TABLES is not None:
        return
    import concourse.hw_specs as hw_specs
    _ORIG_GET_ACT_TABLES = hw_specs.get_activation_tables

    def pinned(arch):
        tabs = dict(_ORIG_GET_ACT_TABLES(arch))
        pref = "natural_log_exp_and_others"
        if pref not in tabs:
            return tabs
        exp_ln = {mybir.ActivationFunctionType.Exp,
                  mybir.ActivationFunctionType.Ln}
        out = {}
        for k, v in tabs.items():
            out[k] = set(v) if k == pref else set(v) - exp_ln
        return out

    bacc.get_activation_tables = pinned


def build_nc():
    _patch_act_tables()
    nc = bacc.Bacc("TRN2", target_bir_lowering=False, debug=False, num_devices=8)

    co_d = nc.declare_dram_parameter("co", [T, C], F32, isOutput=False)
    w_d = nc.declare_dram_parameter("wif", [C, IF], F32, isOutput=False)
    b_d = nc.declare_dram_parameter("bif", [1, IF], F32, isOutput=False)
    m0_d = nc.declare_dram_parameter("mem0", [N, W], F32, isOutput=False)
    ident_d = nc.declare_dram_parameter("ident", [128, 128], F32, isOutput=False)
    out_d = nc.declare_dram_parameter("out", [T, R, W], F32, isOutput=True)

    with tile.TileContext(nc) as tc:
        with (
            nc.allow_low_precision(reason="bf16 link + f32r broadcasts stay"
                                   " within the 2e-2 gate"),
            tc.tile_pool(name="const", bufs=1) as cp,
            tc.tile_pool(name="state", bufs=2) as sp,
            tc.tile_pool(name="work", bufs=2) as wp,
            tc.tile_pool(name="psBig", bufs=1, space="PSUM") as psB,
            tc.tile_pool(name="psMem", bufs=1, space="PSUM") as psM,
            tc.tile_pool(name="psS", bufs=2, space="PSUM") as psS,
        ):
            _build_body(nc, tc, cp, sp, wp, psB, psM, psS,
                        co_d, w_d, b_d, m0_d, ident_d, out_d)
    nc.compile()
    return nc


def _build_body(nc, tc, cp, sp, wp, psB, psM, psS,
                co_d, w_d, b_d, m0_d, ident_d, out_d):
    V, S, P, G_, DMA = nc.vector, nc.scalar, nc.tensor, nc.gpsimd, nc.sync

    # ---------------- constants ----------------
    ident = cp.tile([128, 128], F32)
    DMA.dma_start(ident[:], ident_d[:])
    ones = cp.tile([128, 128], F32)
    G_.memset(ones[:], 1.0)
    ones_b = cp.tile([1, 128], BF16)
    G_.memset(ones_b[:], 1.0)

    # persistent per-t tables
    iface = cp.tile([T, IF], F32)          # raw iface rows
    wvR = cp.tile([1, T, W], BF16)         # write vectors, partition-0 rows
    keysc = cp.tile([W, 5, T], F32)        # scaled keys: r=0..3 read, 4 write
    neg_er = cp.tile([W, T], F32)
    gr = cp.tile([1, 6, T], F32)           # sigmoids: fg x4, ag, wg
    c1p = cp.tile([1, T], F32)
    cn1 = cp.tile([1, T], F32)
    c2 = cp.tile([1, T], F32)
    modes1 = cp.tile([1, R, T], F32)       # content-mode row per t
    mbs0 = cp.tile([128, R, T], F32)
    mbs2 = cp.tile([128, R, T], F32)
    nege0_pm = cp.tile([128, NT], F32)
    G_.memset(nege0_pm[:], 0.0)
    G_.memset(nege0_pm[0:1, 0:1], -1.0)
    out_sb = cp.tile([W, T, R], F32)

    # ---------------- prologue ----------------
    with tc.tile_pool(name="prolog", bufs=1) as pp:
        co_sb = pp.tile([T, C], F32)
        DMA.dma_start(co_sb[:], co_d[:])
        bif_sb = pp.tile([1, IF], F32)
        DMA.dma_start(bif_sb[:], b_d[:])
        w_sb = pp.tile([128, 8, IF], F32)
        for k in range(8):
            # split the 1.9MB load across two hwdge queues
            eng = DMA if k % 2 == 0 else nc.scalar
            eng.dma_start(w_sb[:, k, :], w_d[128 * k:128 * (k + 1), :])

        coT_p = psB.tile([128, 8, T], F32, tag="wb")
        for k in range(8):
            P.transpose(coT_p[:, k, :], co_sb[:, 128 * k:128 * (k + 1)],
                        ident[0:T, 0:T])
        coT = pp.tile([128, 8, T], F32)
        V.tensor_copy(coT[:], coT_p[:])

        if_p = psB.tile([T, IF], F32, tag="pb", bufs=2)
        for k in range(8):
            P.matmul(if_p[:], coT[:, k, :], w_sb[:, k, :],
                     start=(k == 0), stop=False)
        P.matmul(if_p[:], ones[0:1, 0:T], bif_sb[:],
                 start=False, stop=True)
        V.tensor_copy(iface[:], if_p[:])

        # keys [64, 5, T]: read r=0..3, write at 4
        keys_p = psB.tile([W, 5, T], F32, tag="pb", bufs=2)
        for r in range(R):
            P.transpose(keys_p[:, r, :], iface[:, O_RK + W * r:O_RK + W * (r + 1)],
                        ident[0:T, 0:T])
        P.transpose(keys_p[:, 4, :], iface[:, O_WK:O_WK + W], ident[0:T, 0:T])
        keys = pp.tile([W, 5, T], F32)
        V.tensor_copy(keys[:], keys_p[:])

        # write vectors as partition-0 rows via selector matmuls, two copies
        for h in range(2):
            wv_p = psB.tile([1, 8, W], F32, tag="pb", bufs=2, name=f"wvp{h}")
            for j in range(8):
                tt_ = 8 * h + j
                P.matmul(wv_p[0:1, j, :], ident[0:T, tt_:tt_ + 1],
                         iface[:, O_WV:O_WV + W])
            V.tensor_copy(wvR[0:1, 8 * h:8 * (h + 1), :].rearrange(
                "o t w -> o (t w)"),
                wv_p[:].rearrange("o t w -> o (t w)"))

        # erase sigmoid -> neg_er
        er_p = psS.tile([W, T], F32, tag="s")
        P.transpose(er_p[:], iface[:, O_ER:O_ER + W], ident[0:T, 0:T])
        ee = pp.tile([W, T], F32)
        S.activation(ee[:], er_p[:], AF.Exp, scale=-1.0)
        ew = pp.tile([W, T], F32)
        V.tensor_scalar(ew[:], ee[:], 1.0, None, op0=OP.add)
        er_r = pp.tile([W, T], F32)
        V.reciprocal(er_r[:], ew[:])
        V.tensor_scalar(neg_er[:], er_r[:], -1.0, None, op0=OP.mult)

        # strengths softplus: [1, 5, T] (rs x4, ws)
        sts_p = psS.tile([1, 5, T], F32, tag="s")
        for r in range(R):
            P.transpose(sts_p[0:1, r, :], iface[:, O_RS + r:O_RS + r + 1],
                        ident[0:T, 0:T])
        P.transpose(sts_p[0:1, 4, :], iface[:, O_WS:O_WS + 1], ident[0:T, 0:T])
        st_e = pp.tile([1, 5 * T], F32)
        S.activation(st_e[:], sts_p[:].rearrange("o f t -> o (f t)"), AF.Exp)
        st_w = pp.tile([1, 5 * T], F32)
        V.tensor_scalar(st_w[:], st_e[:], 1.0, None, op0=OP.add)
        st_sp = pp.tile([1, 5 * T], F32)
        S.activation(st_sp[:], st_w[:], AF.Ln)

        # key norms: rsqrt(sum keys^2) = exp(-0.5 ln)
        sqk = pp.tile([W, 5 * T], F32)
        S.activation(sqk[:], keys[:].rearrange("w f t -> w (f t)"), AF.Square)
        k2_p = psM.tile([1, 5 * T], F32, tag="wwb")
        P.matmul(k2_p[:], ones[0:W, 0:1], sqk[:])
        lk2 = pp.tile([1, 5 * T], F32)
        S.activation(lk2[:], k2_p[:], AF.Ln)
        kr = pp.tile([1, 5 * T], F32)
        S.activation(kr[:], lk2[:], AF.Exp, scale=-0.5)
        beta = pp.tile([1, 5 * T], F32)
        V.tensor_tensor(beta[:], st_sp[:], kr[:], op=OP.mult)
        kb_p = psM.tile([W, 5 * T], F32, tag="add")
        P.matmul(kb_p[:], ones[0:1, 0:W], beta[:])
        V.tensor_tensor(keysc[:].rearrange("w f t -> w (f t)"),
                        keys[:].rearrange("w f t -> w (f t)"), kb_p[:],
                        op=OP.mult)

        # gates: fg x4, ag, wg sigmoids
        gats_p = psS.tile([1, 6, T], F32, tag="s")
        for r in range(R):
            P.transpose(gats_p[0:1, r, :], iface[:, O_FG + r:O_FG + r + 1],
                        ident[0:T, 0:T])
        P.transpose(gats_p[0:1, 4, :], iface[:, O_AG:O_AG + 1], ident[0:T, 0:T])
        P.transpose(gats_p[0:1, 5, :], iface[:, O_WG:O_WG + 1], ident[0:T, 0:T])
        g_e = pp.tile([1, 6 * T], F32)
        S.activation(g_e[:], gats_p[:].rearrange("o g t -> o (g t)"), AF.Exp,
                     scale=-1.0)
        g_w = pp.tile([1, 6 * T], F32)
        V.tensor_scalar(g_w[:], g_e[:], 1.0, None, op0=OP.add)
        V.reciprocal(gr[:].rearrange("o g t -> o (g t)"), g_w[:])
        ag_t = gr[0:1, 4, :]
        wg_t = gr[0:1, 5, :]
        V.tensor_tensor(c1p[:], ag_t, wg_t, op=OP.mult)
        V.tensor_scalar(cn1[:], c1p[:], -1.0, None, op0=OP.mult)
        V.tensor_tensor(c2[:], wg_t, c1p[:], op=OP.subtract)

        # modes softmax -> rows per t
        me = pp.tile([T, 12], F32)
        S.activation(me[:], iface[:, O_MD:O_MD + 12], AF.Exp)
        me3 = me[:].rearrange("t (r m) -> t r m", m=3)
        msum = pp.tile([T, R], F32)
        V.tensor_tensor(msum[:], me3[:, :, 0], me3[:, :, 1], op=OP.add)
        V.tensor_tensor(msum[:], msum[:], me3[:, :, 2], op=OP.add)
        mrcp = pp.tile([T, R], F32)
        V.reciprocal(mrcp[:], msum[:])
        mn = pp.tile([T, 12], F32)
        mn3 = mn[:].rearrange("t (m r) -> t m r", r=R)
        me3b = me[:].rearrange("t (r m) -> t m r", m=3)
        for m in range(3):
            V.tensor_tensor(mn3[:, m, :], me3b[:, m, :], mrcp[:], op=OP.mult)
        # three m-blocks at base partition 0: modes0/1/2 [4, T]
        mblk_p = psS.tile([R, 3, T], F32, tag="s")
        for m in range(3):
            P.transpose(mblk_p[:, m, :], mn[:, 4 * m:4 * (m + 1)],
                        ident[0:T, 0:T])
        mblk = pp.tile([R, 3, T], F32)
        V.tensor_copy(mblk[:], mblk_p[:])
        m1sel_p = psS.tile([1, R, T], F32, tag="s")
        for r in range(R):
            P.matmul(m1sel_p[0:1, r, :], ident[0:R, r:r + 1], mblk[:, 1, :])
        V.tensor_copy(modes1[:].rearrange("o r t -> o (r t)"),
                      m1sel_p[:].rearrange("o r t -> o (r t)"))
        # flatten rows r of m-block 0/2 onto partition 0 via selector matmuls
        mrows_p = psS.tile([1, 2, R, T], F32, tag="s")
        for r in range(R):
            P.matmul(mrows_p[0:1, 0, r, :], ident[0:R, r:r + 1], mblk[:, 0, :])
            P.matmul(mrows_p[0:1, 1, r, :], ident[0:R, r:r + 1], mblk[:, 2, :])
        mrows = pp.tile([1, 2, R, T], F32)
        V.tensor_copy(mrows[:].rearrange("o a r t -> o (a r t)"),
                      mrows_p[:].rearrange("o a r t -> o (a r t)"))
        mb0_p = psB.tile([128, R * T], F32, tag="wb")
        P.matmul(mb0_p[:], ones[0:1, :], mrows[0:1, 0, :, :])
        V.tensor_copy(mbs0[:].rearrange("p r t -> p (r t)"), mb0_p[:])
        mb2_p = psB.tile([128, R * T], F32, tag="pb", bufs=2)
        P.matmul(mb2_p[:], ones[0:1, :], mrows[0:1, 1, :, :])
        V.tensor_copy(mbs2[:].rearrange("p r t -> p (r t)"), mb2_p[:])

    # ---------------- initial state ----------------
    mem_nrm = sp.tile([128, NT, W], F32, tag="mem_nrm")
    for c in range(NT):
        DMA.dma_start(mem_nrm[:, c, :],
                      m0_d[128 * c:128 * (c + 1), :])
    memT_p = psB.tile([W, N], F32, tag="wb")
    for c in range(NT):
        P.transpose(memT_p[:, 128 * c:128 * (c + 1)],
                    mem_nrm[:, c, :], ident[:])
    memT = sp.tile([W, N], F32, tag="memT")
    V.tensor_copy(memT[:], memT_p[:])

    # initial norm: PM-layout sqn -> msq -> Ln/Exp
    sqn0 = wp.tile([128, NT, W], F32, tag="sqn")
    G_.tensor_tensor(sqn0[:], mem_nrm[:], mem_nrm[:], op=OP.mult)
    msq0 = wp.tile([128, NT], F32, tag="msq")
    V.tensor_reduce(msq0[:], sqn0[:], axis=mybir.AxisListType.X, op=OP.add)
    lms0 = wp.tile([128, NT], F32, tag="lms")
    S.activation(lms0[:], msq0[:], AF.Ln)
    mnorm_i = sp.tile([128, NT], F32, tag="mnorm")
    S.activation(mnorm_i[:], lms0[:], AF.Exp, scale=-0.5)

    L = sp.tile([128, NT, N], BF16, tag="L")
    G_.memset(L[:], 0.0)
    LT0 = sp.tile([128, NT, N], BF16, tag="LT")
    G_.memset(LT0[:], 0.0)
    dL0 = sp.tile([128, NT], F32, tag="dL")
    G_.memset(dL0[:], 0.0)

    st = dict(memT=memT, mem_nrm=mem_nrm, mnorm=mnorm_i, L=L, LT=LT0,
              dL=dL0, u_pm=None, prec_pm=None, prec_fl=None,
              pbs=None, rwT=None)

    for t in range(T):
        st = _step(nc, t, st, cp, sp, wp, psB, psM, psS,
                   ident, ones, ones_b, iface, wvR, keysc, neg_er, gr, c1p,
                   cn1, c2, modes1, mbs0, mbs2, nege0_pm, out_sb)

    DMA.dma_start(out_d[:].rearrange("t r w -> w t r"), out_sb[:])


def _step(nc, t, st, cp, sp, wp, psB, psM, psS,
          ident, ones, ones_b, iface, wvR, keysc, neg_er, gr, c1p, cn1, c2,
          modes1, mbs0, mbs2, nege0_pm, out_sb):
    V, S, P, G_, DMA = nc.vector, nc.scalar, nc.tensor, nc.gpsimd, nc.sync
    memT, mem_nrm, mnorm = st["memT"], st["mem_nrm"], st["mnorm"]
    L, LT, dL, u_pm = st["L"], st["LT"], st["dL"], st["u_pm"]
    prec_pm, prec_fl, pbs, rwT = (st["prec_pm"], st["prec_fl"], st["pbs"],
                                  st["rwT"])
    last = (t == T - 1)

    if t == 0:
        na_pm, nsa = nege0_pm, None
    else:
        na_pm, nsa = st["na_pm"], st["nsa"]

    # ---- pbs broadcast (prev-step prec; runs at step start) ----
    if t > 0:
        pb_p = psB.tile([128, N], F32, tag="pb", bufs=2)
        P.matmul(pb_p[:], ones_b[0:1, :], prec_fl[:])
        pbs = wp.tile([128, N], BF16, tag="pbs")
        S.activation(pbs[:], pb_p[:], AF.Copy)

    # ---- write content softmax (PM) ----
    wdots_p = psS.tile([128, NT], F32, tag="s")
    for b in range(NT):
        P.matmul(wdots_p[:, b:b + 1], memT[:, 128 * b:128 * (b + 1)],
                 keysc[:, 4, t:t + 1])
    wlog = wp.tile([128, NT], F32, tag="wlog")
    V.tensor_tensor(wlog[:], wdots_p[:], mnorm[:], op=OP.mult)
    wexp_pm = wp.tile([128, NT], F32, tag="wexp")
    S.activation(wexp_pm[:], wlog[:], AF.Exp)
    wps_p = psS.tile([1, NT], F32, tag="s")
    P.matmul(wps_p[:], ones[:, 0:1], wexp_pm[:])
    wsum = wp.tile([1, 1], F32, tag="wsum")
    V.tensor_reduce(wsum[:], wps_p[:], axis=mybir.AxisListType.X, op=OP.add)
    wrs = wp.tile([1, 1], F32, tag="wrs")
    V.reciprocal(wrs[:], wsum[:])
    cw = wp.tile([1, 1], F32, tag="cw")
    V.tensor_tensor(cw[:], wrs[:], c2[0:1, t:t + 1], op=OP.mult)

    # ---- ww assembly (PM) ----
    cn1b_p = psS.tile([128, 1], F32, tag="s")
    P.matmul(cn1b_p[:], ones[0:1, :], cn1[0:1, t:t + 1])
    cwb_p = psS.tile([128, 1], F32, tag="s")
    P.matmul(cwb_p[:], ones[0:1, :], cw[:])
    wwx = wp.tile([128, NT], F32, tag="wwx")
    V.tensor_scalar(wwx[:], na_pm[:], cn1b_p[:, 0:1], None, op0=OP.mult)
    ww_pm = wp.tile([128, NT], F32, tag="wwpm")
    V.scalar_tensor_tensor(ww_pm[:], wexp_pm[:], cwb_p[:, 0:1], wwx[:],
                           op0=OP.mult, op1=OP.add)
    if t > 0:
        omw_pm = wp.tile([128, NT], F32, tag="omw")
        V.tensor_scalar(omw_pm[:], ww_pm[:], -1.0, 1.0, op0=OP.mult,
                        op1=OP.add)
    if st.get("fin") is not None:
        st["fin"]()
        st["fin"] = None

    ww_tp = psS.tile([1, N], F32, tag="s")
    for c in range(NT):
        P.transpose(ww_tp[0:1, 128 * c:128 * (c + 1)], ww_pm[:, c:c + 1],
                    ident[:])
    ww_fl = wp.tile([1, N], BF16, tag="wwfl")
    S.copy(ww_fl[:], ww_tp[:])

    # ---- wbs broadcast (bf16, for the link ts ops) ----
    if t > 0:
        wb_p = psB.tile([128, N], F32, tag="wb")
        P.matmul(wb_p[:], ones_b[0:1, :], ww_fl[:])
        wbs = wp.tile([128, N], BF16, tag="wbs")
        S.activation(wbs[:], wb_p[:], AF.Copy)

    # ---- memory head ----
    wwb_p = psM.tile([W, N], F32, tag="wwb")
    P.matmul(wwb_p[:], ones_b[0:1, 0:W], ww_fl[:])
    add_p = psM.tile([W, N], F32, tag="add")
    P.matmul(add_p[:], wvR[0:1, t, :], ww_fl[:])
    keep = wp.tile([W, N], F32, tag="keep")
    S.activation(keep[:], wwb_p[:], AF.Copy, scale=neg_er[:, t:t + 1],
                 bias=1.0)
    m1 = wp.tile([W, N], F32, tag="m1")
    G_.tensor_tensor(m1[:], memT[:], keep[:], op=OP.mult)

    # ---- usage update ----
    if last:
        u_pm_n = u_pm
    else:
        u_pm_n = sp.tile([128, NT], F32, tag="u_pm")
        if t == 0:
            V.tensor_copy(u_pm_n[:], ww_pm[:])
        else:
            fgb_p = psS.tile([128, R], F32, tag="s")
            P.matmul(fgb_p[:], ones[0:1, :], gr[0:1, 0:R, t])
            yyT = wp.tile([128, NT, R], F32, tag="yyT")
            V.scalar_tensor_tensor(
                yyT[:], fgb_p[:, None, :].broadcast_to([128, NT, R]), -1.0,
                rwT[:].rearrange("p (c r) -> p c r", r=R),
                op0=OP.mult, op1=OP.mult)
            om = wp.tile([128, NT, R], F32, tag="om")
            V.tensor_scalar(om[:], yyT[:], 1.0, None, op0=OP.add)
            p1u = wp.tile([128, NT], F32, tag="p1u")
            G_.tensor_tensor(p1u[:], om[:, :, 0], om[:, :, 1], op=OP.mult)
            p2u = wp.tile([128, NT], F32, tag="p2u")
            G_.tensor_tensor(p2u[:], om[:, :, 2], om[:, :, 3], op=OP.mult)
            psi = wp.tile([128, NT], F32, tag="psi")
            G_.tensor_tensor(psi[:], p1u[:], p2u[:], op=OP.mult)
            omu = wp.tile([128, NT], F32, tag="omu")
            V.tensor_scalar(omu[:], u_pm[:], -1.0, 1.0, op0=OP.mult,
                            op1=OP.add)
            tn = wp.tile([128, NT], F32, tag="tn")
            V.scalar_tensor_tensor(tn[:], ww_pm[:], 1.0, omu[:],
                                   op0=OP.subtract, op1=OP.mult)
            V.scalar_tensor_tensor(u_pm_n[:], tn[:], 1.0, psi[:],
                                   op0=OP.add, op1=OP.mult)

    # ---- allocation compare inputs (flat u + broadcast; exact fp32) ----
    if not last:
        u_tp = psS.tile([1, N], F32, tag="s")
        for c in range(NT):
            P.transpose(u_tp[0:1, 128 * c:128 * (c + 1)], u_pm_n[:, c:c + 1],
                        ident[:])
        u_fl_n = wp.tile([1, N], F32, tag="ufl")
        V.tensor_copy(u_fl_n[:], u_tp[:])
        ub_p = psM.tile([128, N], F32, tag="wwb")
        P.matmul(ub_p[:], ones[0:1, :], u_fl_n[:])
        ubs = wp.tile([128, N], F32, tag="ubs")
        S.copy(ubs[:], ub_p[:])
        ucl = wp.tile([128, NT], F32, tag="ucl")
        V.tensor_scalar(ucl[:], u_pm_n[:], 1e-38, None, op0=OP.max)
        lnu = wp.tile([128, NT], F32, tag="lnu")
        S.activation(lnu[:], ucl[:], AF.Ln)

    # ---- prec update ----
    if not last:
        prec_pm_n = sp.tile([128, NT], F32, tag="prec_pm")
        if t == 0:
            V.tensor_copy(prec_pm_n[:], ww_pm[:])
        else:
            swa = wp.tile([1, 1], F32, tag="swa")
            G_.tensor_tensor(swa[:], nsa[:], cn1[0:1, t:t + 1], op=OP.mult)
            sw = wp.tile([1, 1], F32, tag="sw")
            G_.tensor_tensor(sw[:], swa[:], c2[0:1, t:t + 1], op=OP.add)
            omsw = wp.tile([1, 1], F32, tag="omsw")
            V.tensor_scalar(omsw[:], sw[:], -1.0, 1.0, op0=OP.mult,
                            op1=OP.add)
            omsw_p = psS.tile([128, 1], F32, tag="s")
            P.matmul(omsw_p[:], ones[0:1, :], omsw[:])
            V.scalar_tensor_tensor(prec_pm_n[:], prec_pm[:], omsw_p[:, 0:1],
                                   ww_pm[:], op0=OP.mult, op1=OP.add)
        p_tp = psS.tile([1, N], F32, tag="s")
        for c in range(NT):
            P.transpose(p_tp[0:1, 128 * c:128 * (c + 1)], prec_pm_n[:, c:c + 1],
                        ident[:])
        prec_fl_n = sp.tile([1, N], BF16, tag="prec_fl")
        S.copy(prec_fl_n[:], p_tp[:])
    else:
        prec_pm_n, prec_fl_n = prec_pm, prec_fl

    # ---- mode-scaled read weights + link diagonal tracker ----

    # ---- link loop with interleaved memT_n / Gt compares ----
    comb_eng = [(G_, G_), (G_, V), (G_, V), (G_, V)]
    if t == 0:
        L_n, LT_n = L, LT
        memT_n = sp.tile([W, N], F32, tag="memT")
        V.tensor_tensor(memT_n[:], m1[:], add_p[:], op=OP.add)
        if not last:
            Gt_n = wp.tile([128, NT, N], F32, tag="G", bufs=1)
            for c in range(NT):
                V.tensor_scalar(Gt_n[:, c, :], ubs[:], u_pm_n[:, c:c + 1],
                                None, op0=OP.is_gt)
    else:
        L_n = sp.tile([128, NT, N], BF16, tag="L")
        LT_n = sp.tile([128, NT, N], BF16, tag="LT")
        memT_n = sp.tile([W, N], F32, tag="memT")
        if not last:
            Gt_n = wp.tile([128, NT, N], F32, tag="G", bufs=1)
        for c in range(NT):
            w1 = wp.tile([128, N], BF16, tag=f"w1_{c % 2}")
            V.tensor_scalar(w1[:], wbs[:], omw_pm[:, c:c + 1], None,
                            op0=OP.subtract)
            p1 = wp.tile([128, N], BF16, tag=f"p1_{c % 2}")
            V.tensor_scalar(p1[:], pbs[:], ww_pm[:, c:c + 1], None,
                            op0=OP.mult)
            p1T = wp.tile([128, N], BF16, tag=f"p1T_{c % 2}")
            V.tensor_scalar(p1T[:], wbs[:], prec_pm[:, c:c + 1], None,
                            op0=OP.mult)
            t1 = wp.tile([128, N], BF16, tag=f"t1_{c % 2}")
            G_.tensor_tensor(t1[:], w1[:], L[:, c, :], op=OP.mult)
            t1T = wp.tile([128, N], BF16, tag=f"t1T_{c % 2}")
            G_.tensor_tensor(t1T[:], w1[:], LT[:, c, :], op=OP.mult)
            eL, eLT = comb_eng[c]
            eL.tensor_tensor(L_n[:, c, :], p1[:], t1[:], op=OP.subtract)
            eLT.tensor_tensor(LT_n[:, c, :], p1T[:], t1T[:], op=OP.subtract)
            if c == 0:
                V.tensor_tensor(memT_n[:], m1[:], add_p[:], op=OP.add)
            elif not last:
                cc = c - 1
                V.tensor_scalar(Gt_n[:, cc, :], ubs[:],
                                u_pm_n[:, cc:cc + 1], None, op0=OP.is_gt)
        if not last:
            V.tensor_scalar(Gt_n[:, 3, :], ubs[:], u_pm_n[:, 3:4],
                            None, op0=OP.is_gt)

    if t > 0:
        wp_pm = wp.tile([128, NT], F32, tag="wppm")
        G_.tensor_tensor(wp_pm[:], ww_pm[:], prec_pm[:], op=OP.mult)
        dmul = wp.tile([128, NT], F32, tag="dmul")
        V.tensor_scalar(dmul[:], ww_pm[:], -2.0, 1.0, op0=OP.mult, op1=OP.add)
        dL_n = sp.tile([128, NT], F32, tag="dL")
        V.scalar_tensor_tensor(dL_n[:], dL[:], 1.0, dmul[:],
                               op0=OP.mult, op1=OP.mult)
        G_.tensor_tensor(dL_n[:], dL_n[:], wp_pm[:], op=OP.add)
    else:
        dL_n = dL
    if t > 0:
        rwTm0 = wp.tile([128, NT * R], BF16, tag="rwTm0")
        V.tensor_tensor(rwTm0[:].rearrange("p (c r) -> p c r", r=R),
                        rwT[:].rearrange("p (c r) -> p c r", r=R),
                        mbs0[:, None, :, t].broadcast_to([128, NT, R]),
                        op=OP.mult)
        rwTm2 = wp.tile([128, NT * R], BF16, tag="rwTm2")
        V.tensor_tensor(rwTm2[:].rearrange("p (c r) -> p c r", r=R),
                        rwT[:].rearrange("p (c r) -> p c r", r=R),
                        mbs2[:, None, :, t].broadcast_to([128, NT, R]),
                        op=OP.mult)

    # ---- memory norm chain ----
    mem_nrm_p = psS.tile([128, NT, W], F32, tag="mn", bufs=1)
    for c in range(NT):
        P.transpose(mem_nrm_p[:, c, :], memT_n[:, 128 * c:128 * (c + 1)],
                    ident[0:W, 0:W])
    sqn = wp.tile([128, NT, W], F32, tag="sqn")
    S.activation(sqn[:], mem_nrm_p[:], AF.Square)
    msq = wp.tile([128, NT], F32, tag="msq")
    V.tensor_reduce(msq[:], sqn[:], axis=mybir.AxisListType.X, op=OP.add)
    mem_nrm_n = sp.tile([128, NT, W], F32, tag="mem_nrm")
    S.copy(mem_nrm_n[:], mem_nrm_p[:])
    lms = wp.tile([128, NT], F32, tag="lms")
    S.activation(lms[:], msq[:], AF.Ln)
    mnorm_n = sp.tile([128, NT], F32, tag="mnorm")
    S.activation(mnorm_n[:], lms[:], AF.Exp, scale=-0.5)

    # ---- allocation log-sum ----
    if last:
        na_pm_n, nsa_n = None, None
    else:
        sT_p = psS.tile([128, NT], F32, tag="s")
        for b in range(NT):
            for c in range(NT):
                P.matmul(sT_p[:, b:b + 1], Gt_n[:, c, 128 * b:128 * (b + 1)],
                         lnu[:, c:c + 1], start=(c == 0),
                         stop=(c == NT - 1))
        es_pm = wp.tile([128, NT], F32, tag="espm")
        S.activation(es_pm[:], sT_p[:], AF.Exp)
        na_pm_n = wp.tile([128, NT], F32, tag="napm")
        if t < T - 2:
            nap = wp.tile([128, 1], F32, tag="nap")
            V.scalar_tensor_tensor(na_pm_n[:], u_pm_n[:], 1.0, es_pm[:],
                                   op0=OP.subtract, op1=OP.mult,
                                   accum_out=nap[:])
        else:
            V.scalar_tensor_tensor(na_pm_n[:], u_pm_n[:], 1.0, es_pm[:],
                                   op0=OP.subtract, op1=OP.mult)
        if t < T - 2:
            nsa_p = psS.tile([1, 1], F32, tag="s")
            P.matmul(nsa_p[:], nap[:], ones[:, 0:1])
            nsa_n = wp.tile([1, 1], F32, tag="nsa")
            V.tensor_copy(nsa_n[:], nsa_p[:])
        else:
            nsa_n = None

    # ---- read content (PM) ----
    rdots_p = psS.tile([128, NT * R], F32, tag="s")
    for b in range(NT):
        P.matmul(rdots_p[:, R * b:R * (b + 1)],
                 memT_n[:, 128 * b:128 * (b + 1)], keysc[:, 0:4, t])
    rlog = wp.tile([128, NT, R], F32, tag="rlog")
    V.tensor_tensor(rlog[:],
                    rdots_p[:].rearrange("p (c r) -> p c r", r=R),
                    mnorm_n[:, :, None].broadcast_to([128, NT, R]),
                    op=OP.mult)
    rexp_pm = wp.tile([128, NT * R], F32, tag="rexp")
    S.activation(rexp_pm[:], rlog[:].rearrange("p c r -> p (c r)"), AF.Exp)
    rps_p = psS.tile([1, NT * R], F32, tag="s")
    P.matmul(rps_p[:], ones[:, 0:1], rexp_pm[:])
    rsum = wp.tile([1, R], F32, tag="rsum")
    V.tensor_reduce(rsum[:], rps_p[:].rearrange("o (c r) -> o r c", r=R),
                    axis=mybir.AxisListType.X, op=OP.add)
    rsr = wp.tile([1, R], F32, tag="rsr")
    V.reciprocal(rsr[:], rsum[:])
    s1c = wp.tile([1, R], F32, tag="s1c")
    V.tensor_tensor(s1c[:], rsr[:], modes1[0:1, :, t], op=OP.mult)
    s1cb_p = psS.tile([128, R], F32, tag="s")
    P.matmul(s1cb_p[:], ones[0:1, :], s1c[:])

    cnt = wp.tile([128, NT, R], F32, tag="cnt")
    V.tensor_tensor(cnt[:], rexp_pm[:].rearrange("p (c r) -> p c r", r=R),
                    s1cb_p[:, None, :].broadcast_to([128, NT, R]), op=OP.mult)
    rwT_n = sp.tile([128, NT * R], F32, tag="rwT")
    if t > 0:
        s02 = wp.tile([128, NT, R], F32, tag="s02")
        G_.tensor_tensor(s02[:], rwTm0[:].rearrange("p (c r) -> p c r", r=R),
                        rwTm2[:].rearrange("p (c r) -> p c r", r=R),
                        op=OP.add)
        corr = wp.tile([128, NT, R], F32, tag="corr")
        V.tensor_tensor(corr[:], s02[:],
                        dL_n[:, :, None].broadcast_to([128, NT, R]),
                        op=OP.mult)
        cnt2 = wp.tile([128, NT, R], F32, tag="cnt2")
        G_.tensor_tensor(cnt2[:], cnt[:], corr[:], op=OP.subtract)
        def _fin(rwT_n=rwT_n, cnt2=cnt2, mem_nrm_n=mem_nrm_n,
                 L_n=L_n, LT_n=LT_n, rwTm0=rwTm0, rwTm2=rwTm2, t=t):
            rwT_p = psS.tile([128, NT * R], F32, tag="s")
            for b in range(NT):
                blk = slice(128 * b, 128 * (b + 1))
                for c in range(NT):
                    P.matmul(rwT_p[:, R * b:R * (b + 1)], L_n[:, c, blk],
                             rwTm0[:, R * c:R * (c + 1)],
                             start=(c == 0), stop=False)
                for c in range(NT):
                    P.matmul(rwT_p[:, R * b:R * (b + 1)], LT_n[:, c, blk],
                             rwTm2[:, R * c:R * (c + 1)],
                             start=False, stop=(c == NT - 1))
            V.tensor_tensor(rwT_n[:], cnt2[:].rearrange("p c r -> p (c r)"),
                            rwT_p[:], op=OP.add)
            rwd_p = psS.tile([W, R], F32, tag="s")
            for c in range(NT):
                P.matmul(rwd_p[:], mem_nrm_n[:, c, :],
                         rwT_n[:, R * c:R * (c + 1)],
                         start=(c == 0), stop=(c == NT - 1))
            S.copy(out_sb[:, t, :], rwd_p[:])
    else:
        V.tensor_copy(rwT_n[:], cnt[:].rearrange("p c r -> p (c r)"))
        def _fin(rwT_n=rwT_n, mem_nrm_n=mem_nrm_n, t=t):
            rwd_p = psS.tile([W, R], F32, tag="s")
            for c in range(NT):
                P.matmul(rwd_p[:], mem_nrm_n[:, c, :],
                         rwT_n[:, R * c:R * (c + 1)],
                         start=(c == 0), stop=(c == NT - 1))
            S.copy(out_sb[:, t, :], rwd_p[:])
    if last:
        _fin()
    else:
        st_fin = _fin

    return dict(memT=memT_n, mem_nrm=mem_nrm_n, mnorm=mnorm_n, L=L_n,
                LT=LT_n, dL=dL_n, u_pm=u_pm_n, na_pm=na_pm_n,
                nsa=nsa_n, prec_pm=prec_pm_n, prec_fl=prec_fl_n, pbs=pbs,
                rwT=rwT_n, fin=(None if last else st_fin))


# ---------------------------------------------------------------------------
_NC_CACHE = {}


def _get_nc():
    if "nc" not in _NC_CACHE:
        _NC_CACHE["nc"] = build_nc()
    return _NC_CACHE["nc"]


def _consts():
    ident = np.eye(128, dtype=np.float32)
    return (ident,)


def make_in_maps(controller_output, W_if, b_if, memory0):
    (ident,) = _consts()
    maps = []
    for b in range(B):
        maps.append({
            "co": np.ascontiguousarray(controller_output[b]),
            "wif": np.ascontiguousarray(W_if),
            "bif": np.ascontiguousarray(b_if.reshape(1, IF)),
            "mem0": np.ascontiguousarray(memory0[b]),
            "ident": ident,
        })
    return maps


def kernel(controller_output, W_if, b_if, memory0):
    from concourse.bass_utils import run_bass_kernel_spmd
    controller_output = np.asarray(controller_output, dtype=np.float32)
    W_if = np.asarray(W_if, dtype=np.float32)
    b_if = np.asarray(b_if, dtype=np.float32)
    memory0 = np.asarray(memory0, dtype=np.float32)
    nc = _get_nc()
    maps = make_in_maps(controller_output, W_if, b_if, memory0)
    # Retry once on non-finite output: a stale device (e.g. after an
    # earlier aborted run) can poison cores on the first dispatch.
    for _ in range(2):
        res = run_bass_kernel_spmd(nc, maps, core_ids=list(range(B)))
        out = np.stack([res.results[b]["out"] for b in range(B)], axis=0)
        if np.isfinite(out).all():
            break
    return out


if __name__ == "__main__":
    mode = sys.argv[1] if len(sys.argv) > 1 else "sim"
    sys.path.insert(0, "/root/problem")
    import jax
    with jax.default_device(jax.devices("cpu")[0]):
        import reference
        inputs = {k: np.asarray(v) for k, v in reference.setup_inputs().items()}
        expected = np.asarray(reference.reference(**inputs))

    if mode == "sim":
        from concourse.bass_interp import CoreSim
        nc = build_nc()
        maps = make_in_maps(inputs["controller_output"], inputs["W_if"],
                            inputs["b_if"], inputs["memory0"])
        sim = CoreSim(nc)
        for k, v in maps[0].items():
            sim.tensor(k)[:] = v
        sim.simulate()
        got = sim.tensor("out").copy()
        exp = expected[0]
        err = np.abs(got - exp)
        rel = np.linalg.norm(got - exp) / (np.linalg.norm(exp) + 1e-12)
        print("sim modeled time (ns):", sim.time)
        print("max abs err:", err.max(), " rel err:", rel)
    else:
        got = kernel(**inputs)
        rel = np.linalg.norm(got - expected) / (np.linalg.norm(expected) + 1e-12)
        print("max abs err:", np.abs(got - expected).max(), " rel err:", rel)


# revision 14
# speedup vs baseline: 1.7540x; 1.0068x over previous
"""DNC MemoryAccess kernel for Trainium2 (Bass/Tile), data-parallel over batch.

Shapes (hardcoded): B=8, T=16, C=1024, IFACE=471, N=512, WORD=64, R=4, NW=1.
Each of the 8 cores processes one batch element; all recurrent state stays
SBUF-resident across the T=16 sequential steps.

Design (vs the fp32 predecessor, 326us -> 192us modeled):
- the temporal link matrix L and its transpose LT are held in bf16; the
  elementwise recurrence L' = (1-w_i-w_j)L + w_i p_j runs as fast-mode
  tensor_scalar ops (0.25x DVE cycles in bf16) for w1 = w_j-(1-w_i) and the
  rank-1 terms, with the tensor_tensor multiplies/combines split across
  Pool and DVE,
- the link diagonal is never zeroed in-place: the scalar recurrence
  d' = (1-2w)d + w p is tracked in [128,NT] and its contribution is
  subtracted from the fwd/bwd PE matmul results,
- broadcast matmuls (ww, prec over partitions/words) use bf16 operands
  (1 PE cycle/row vs 4 for fp32); the usage broadcast for the allocation
  sort compare stays exact fp32 so sort ties match the fp32 reference,
- ln(usage) for the allocation cumprod and the memory-norm rsqrt use the
  Activation-table Ln/Exp; get_activation_tables is patched (membership
  only, original set order preserved) so Exp and Ln resolve to the one
  act-func set that contains both, hoisting the 1.3us table load out of
  the step loop,
- emission order is tuned for the per-engine in-order queues: the read
  softmax, rwTm scaling and dL tracker are emitted so the DVE queue never
  head-blocks the ww chain of the next step; the precedence flat vector
  is produced by PE transposes + one Act copy,
- float32r matmuls are NOT used: they fail neuronxcc BIR verification in
  this toolchain (sim accepts them; hardware compile rejects).

Precision: bf16 rounds the link matrices and the write/erase broadcasts
(~1e-3 relative on the output); usage comparisons stay exact fp32 so the
allocation sort matches the reference except for genuine fp32 ties (b=7
carries one, same as the fp32 baseline).
"""
import sys

sys.path.insert(0, "/opt/trn_rl_repo")

import numpy as np

import concourse.bacc as bacc
import concourse.bass as bass
import concourse.mybir as mybir
import concourse.tile as tile

F32 = mybir.dt.float32
F32R = mybir.dt.float32r
BF16 = mybir.dt.bfloat16
I32 = mybir.dt.int32
AF = mybir.ActivationFunctionType
OP = mybir.AluOpType

B, T, C, IF = 8, 16, 1024, 471
N, W, R = 512, 64, 4
NT = N // 128

O_RK, O_RS, O_WK, O_WS = 0, 256, 260, 324
O_ER, O_WV, O_FG, O_AG, O_WG, O_MD = 325, 389, 453, 457, 458, 459


def fr(ap):
    return ap


# Prefer the activation-function set that contains Exp AND Ln (plus
# Copy/Square/Sign), so the per-step Exp/Ln mix resolves to one table and the
# compiler hoists a single LoadActFuncSet out of the step loop instead of
# thrashing 1283ns loads between exp-only and ln-only sets.
_ORIG_GET_ACT_TABLES = None


def _patch_act_tables():
    global _ORIG_GET_ACT_TABLES
    if _ORIG_GET_ACT_TABLES is not None:
        return
    import concourse.hw_specs as hw_specs
    _ORIG_GET_ACT_TABLES = hw_specs.get_activation_tables

    def pinned(arch):
        tabs = dict(_ORIG_GET_ACT_TABLES(arch))
        pref = "natural_log_exp_and_others"
        if pref not in tabs:
            return tabs
        exp_ln = {mybir.ActivationFunctionType.Exp,
                  mybir.ActivationFunctionType.Ln}
        out = {}
        for k, v in tabs.items():
            out[k] = set(v) if k == pref else set(v) - exp_ln
        return out

    bacc.get_activation_tables = pinned


def build_nc():
    _patch_act_tables()
    nc = bacc.Bacc("TRN2", target_bir_lowering=False, debug=False, num_devices=8)

    co_d = nc.declare_dram_parameter("co", [T, C], F32, isOutput=False)
    w_d = nc.declare_dram_parameter("wif", [C, IF], F32, isOutput=False)
    b_d = nc.declare_dram_parameter("bif", [1, IF], F32, isOutput=False)
    m0_d = nc.declare_dram_parameter("mem0", [N, W], F32, isOutput=False)
    ident_d = nc.declare_dram_parameter("ident", [128, 128], F32, isOutput=False)
    out_d = nc.declare_dram_parameter("out", [T, R, W], F32, isOutput=True)

    with tile.TileContext(nc) as tc:
        with (
            nc.allow_low_precision(reason="bf16 link + f32r broadcasts stay"
                                   " within the 2e-2 gate"),
            tc.tile_pool(name="const", bufs=1) as cp,
            tc.tile_pool(name="state", bufs=2) as sp,
            tc.tile_pool(name="work", bufs=2) as wp,
            tc.tile_pool(name="psBig", bufs=1, space="PSUM") as psB,
            tc.tile_pool(name="psMem", bufs=1, space="PSUM") as psM,
            tc.tile_pool(name="psS", bufs=2, space="PSUM") as psS,
        ):
            _build_body(nc, tc, cp, sp, wp, psB, psM, psS,
                        co_d, w_d, b_d, m0_d, ident_d, out_d)
    nc.compile()
    return nc


def _build_body(nc, tc, cp, sp, wp, psB, psM, psS,
                co_d, w_d, b_d, m0_d, ident_d, out_d):
    V, S, P, G_, DMA = nc.vector, nc.scalar, nc.tensor, nc.gpsimd, nc.sync

    # ---------------- constants ----------------
    ident = cp.tile([128, 128], F32)
    DMA.dma_start(ident[:], ident_d[:])
    ones = cp.tile([128, 128], F32)
    G_.memset(ones[:], 1.0)
    ones_b = cp.tile([1, 128], BF16)
    G_.memset(ones_b[:], 1.0)

    # persistent per-t tables
    iface = cp.tile([T, IF], F32)          # raw iface rows
    wvR = cp.tile([1, T, W], BF16)         # write vectors, partition-0 rows
    keysc = cp.tile([W, 5, T], F32)        # scaled keys: r=0..3 read, 4 write
    neg_er = cp.tile([W, T], F32)
    gr = cp.tile([1, 6, T], F32)           # sigmoids: fg x4, ag, wg
    c1p = cp.tile([1, T], F32)
    cn1 = cp.tile([1, T], F32)
    c2 = cp.tile([1, T], F32)
    modes1 = cp.tile([1, R, T], F32)       # content-mode row per t
    mbs0 = cp.tile([128, R, T], F32)
    mbs2 = cp.tile([128, R, T], F32)
    nege0_pm = cp.tile([128, NT], F32)
    G_.memset(nege0_pm[:], 0.0)
    G_.memset(nege0_pm[0:1, 0:1], -1.0)
    out_sb = cp.tile([W, T, R], F32)

    # ---------------- prologue ----------------
    with tc.tile_pool(name="prolog", bufs=1) as pp:
        co_sb = pp.tile([T, C], F32)
        DMA.dma_start(co_sb[:], co_d[:])
        bif_sb = pp.tile([1, IF], F32)
        DMA.dma_start(bif_sb[:], b_d[:])
        w_sb = pp.tile([128, 8, IF], F32)
        for k in range(8):
            # split the 1.9MB load across two hwdge queues
            eng = DMA if k % 2 == 0 else nc.scalar
            eng.dma_start(w_sb[:, k, :], w_d[128 * k:128 * (k + 1), :])

        coT_p = psB.tile([128, 8, T], F32, tag="wb")
        for k in range(8):
            P.transpose(coT_p[:, k, :], co_sb[:, 128 * k:128 * (k + 1)],
                        ident[0:T, 0:T])
        coT = pp.tile([128, 8, T], F32)
        V.tensor_copy(coT[:], coT_p[:])

        if_p = psB.tile([T, IF], F32, tag="pb", bufs=2)
        for k in range(8):
            P.matmul(if_p[:], coT[:, k, :], w_sb[:, k, :],
                     start=(k == 0), stop=False)
        P.matmul(if_p[:], ones[0:1, 0:T], bif_sb[:],
                 start=False, stop=True)
        V.tensor_copy(iface[:], if_p[:])

        # keys [64, 5, T]: read r=0..3, write at 4
        keys_p = psB.tile([W, 5, T], F32, tag="pb", bufs=2)
        for r in range(R):
            P.transpose(keys_p[:, r, :], iface[:, O_RK + W * r:O_RK + W * (r + 1)],
                        ident[0:T, 0:T])
        P.transpose(keys_p[:, 4, :], iface[:, O_WK:O_WK + W], ident[0:T, 0:T])
        keys = pp.tile([W, 5, T], F32)
        V.tensor_copy(keys[:], keys_p[:])

        # write vectors as partition-0 rows via selector matmuls, two copies
        for h in range(2):
            wv_p = psB.tile([1, 8, W], F32, tag="pb", bufs=2, name=f"wvp{h}")
            for j in range(8):
                tt_ = 8 * h + j
                P.matmul(wv_p[0:1, j, :], ident[0:T, tt_:tt_ + 1],
                         iface[:, O_WV:O_WV + W])
            V.tensor_copy(wvR[0:1, 8 * h:8 * (h + 1), :].rearrange(
                "o t w -> o (t w)"),
                wv_p[:].rearrange("o t w -> o (t w)"))

        # erase sigmoid -> neg_er
        er_p = psS.tile([W, T], F32, tag="s")
        P.transpose(er_p[:], iface[:, O_ER:O_ER + W], ident[0:T, 0:T])
        ee = pp.tile([W, T], F32)
        S.activation(ee[:], er_p[:], AF.Exp, scale=-1.0)
        ew = pp.tile([W, T], F32)
        V.tensor_scalar(ew[:], ee[:], 1.0, None, op0=OP.add)
        er_r = pp.tile([W, T], F32)
        V.reciprocal(er_r[:], ew[:])
        V.tensor_scalar(neg_er[:], er_r[:], -1.0, None, op0=OP.mult)

        # strengths softplus: [1, 5, T] (rs x4, ws)
        sts_p = psS.tile([1, 5, T], F32, tag="s")
        for r in range(R):
            P.transpose(sts_p[0:1, r, :], iface[:, O_RS + r:O_RS + r + 1],
                        ident[0:T, 0:T])
        P.transpose(sts_p[0:1, 4, :], iface[:, O_WS:O_WS + 1], ident[0:T, 0:T])
        st_e = pp.tile([1, 5 * T], F32)
        S.activation(st_e[:], sts_p[:].rearrange("o f t -> o (f t)"), AF.Exp)
        st_w = pp.tile([1, 5 * T], F32)
        V.tensor_scalar(st_w[:], st_e[:], 1.0, None, op0=OP.add)
        st_sp = pp.tile([1, 5 * T], F32)
        S.activation(st_sp[:], st_w[:], AF.Ln)

        # key norms: rsqrt(sum keys^2) = exp(-0.5 ln)
        sqk = pp.tile([W, 5 * T], F32)
        S.activation(sqk[:], keys[:].rearrange("w f t -> w (f t)"), AF.Square)
        k2_p = psM.tile([1, 5 * T], F32, tag="wwb")
        P.matmul(k2_p[:], ones[0:W, 0:1], sqk[:])
        lk2 = pp.tile([1, 5 * T], F32)
        S.activation(lk2[:], k2_p[:], AF.Ln)
        kr = pp.tile([1, 5 * T], F32)
        S.activation(kr[:], lk2[:], AF.Exp, scale=-0.5)
        beta = pp.tile([1, 5 * T], F32)
        V.tensor_tensor(beta[:], st_sp[:], kr[:], op=OP.mult)
        kb_p = psM.tile([W, 5 * T], F32, tag="add")
        P.matmul(kb_p[:], ones[0:1, 0:W], beta[:])
        V.tensor_tensor(keysc[:].rearrange("w f t -> w (f t)"),
                        keys[:].rearrange("w f t -> w (f t)"), kb_p[:],
                        op=OP.mult)

        # gates: fg x4, ag, wg sigmoids
        gats_p = psS.tile([1, 6, T], F32, tag="s")
        for r in range(R):
            P.transpose(gats_p[0:1, r, :], iface[:, O_FG + r:O_FG + r + 1],
                        ident[0:T, 0:T])
        P.transpose(gats_p[0:1, 4, :], iface[:, O_AG:O_AG + 1], ident[0:T, 0:T])
        P.transpose(gats_p[0:1, 5, :], iface[:, O_WG:O_WG + 1], ident[0:T, 0:T])
        g_e = pp.tile([1, 6 * T], F32)
        S.activation(g_e[:], gats_p[:].rearrange("o g t -> o (g t)"), AF.Exp,
                     scale=-1.0)
        g_w = pp.tile([1, 6 * T], F32)
        V.tensor_scalar(g_w[:], g_e[:], 1.0, None, op0=OP.add)
        V.reciprocal(gr[:].rearrange("o g t -> o (g t)"), g_w[:])
        ag_t = gr[0:1, 4, :]
        wg_t = gr[0:1, 5, :]
        V.tensor_tensor(c1p[:], ag_t, wg_t, op=OP.mult)
        V.tensor_scalar(cn1[:], c1p[:], -1.0, None, op0=OP.mult)
        V.tensor_tensor(c2[:], wg_t, c1p[:], op=OP.subtract)

        # modes softmax -> rows per t
        me = pp.tile([T, 12], F32)
        S.activation(me[:], iface[:, O_MD:O_MD + 12], AF.Exp)
        me3 = me[:].rearrange("t (r m) -> t r m", m=3)
        msum = pp.tile([T, R], F32)
        V.tensor_tensor(msum[:], me3[:, :, 0], me3[:, :, 1], op=OP.add)
        V.tensor_tensor(msum[:], msum[:], me3[:, :, 2], op=OP.add)
        mrcp = pp.tile([T, R], F32)
        V.reciprocal(mrcp[:], msum[:])
        mn = pp.tile([T, 12], F32)
        mn3 = mn[:].rearrange("t (m r) -> t m r", r=R)
        me3b = me[:].rearrange("t (r m) -> t m r", m=3)
        for m in range(3):
            V.tensor_tensor(mn3[:, m, :], me3b[:, m, :], mrcp[:], op=OP.mult)
        # three m-blocks at base partition 0: modes0/1/2 [4, T]
        mblk_p = psS.tile([R, 3, T], F32, tag="s")
        for m in range(3):
            P.transpose(mblk_p[:, m, :], mn[:, 4 * m:4 * (m + 1)],
                        ident[0:T, 0:T])
        mblk = pp.tile([R, 3, T], F32)
        V.tensor_copy(mblk[:], mblk_p[:])
        m1sel_p = psS.tile([1, R, T], F32, tag="s")
        for r in range(R):
            P.matmul(m1sel_p[0:1, r, :], ident[0:R, r:r + 1], mblk[:, 1, :])
        V.tensor_copy(modes1[:].rearrange("o r t -> o (r t)"),
                      m1sel_p[:].rearrange("o r t -> o (r t)"))
        # flatten rows r of m-block 0/2 onto partition 0 via selector matmuls
        mrows_p = psS.tile([1, 2, R, T], F32, tag="s")
        for r in range(R):
            P.matmul(mrows_p[0:1, 0, r, :], ident[0:R, r:r + 1], mblk[:, 0, :])
            P.matmul(mrows_p[0:1, 1, r, :], ident[0:R, r:r + 1], mblk[:, 2, :])
        mrows = pp.tile([1, 2, R, T], F32)
        V.tensor_copy(mrows[:].rearrange("o a r t -> o (a r t)"),
                      mrows_p[:].rearrange("o a r t -> o (a r t)"))
        mb0_p = psB.tile([128, R * T], F32, tag="wb")
        P.matmul(mb0_p[:], ones[0:1, :], mrows[0:1, 0, :, :])
        V.tensor_copy(mbs0[:].rearrange("p r t -> p (r t)"), mb0_p[:])
        mb2_p = psB.tile([128, R * T], F32, tag="pb", bufs=2)
        P.matmul(mb2_p[:], ones[0:1, :], mrows[0:1, 1, :, :])
        V.tensor_copy(mbs2[:].rearrange("p r t -> p (r t)"), mb2_p[:])

    # ---------------- initial state ----------------
    mem_nrm = sp.tile([128, NT, W], F32, tag="mem_nrm")
    for c in range(NT):
        DMA.dma_start(mem_nrm[:, c, :],
                      m0_d[128 * c:128 * (c + 1), :])
    memT_p = psB.tile([W, N], F32, tag="wb")
    for c in range(NT):
        P.transpose(memT_p[:, 128 * c:128 * (c + 1)],
                    mem_nrm[:, c, :], ident[:])
    memT = sp.tile([W, N], F32, tag="memT")
    V.tensor_copy(memT[:], memT_p[:])

    # initial norm: PM-layout sqn -> msq -> Ln/Exp
    sqn0 = wp.tile([128, NT, W], F32, tag="sqn")
    G_.tensor_tensor(sqn0[:], mem_nrm[:], mem_nrm[:], op=OP.mult)
    msq0 = wp.tile([128, NT], F32, tag="msq")
    V.tensor_reduce(msq0[:], sqn0[:], axis=mybir.AxisListType.X, op=OP.add)
    lms0 = wp.tile([128, NT], F32, tag="lms")
    S.activation(lms0[:], msq0[:], AF.Ln)
    mnorm_i = sp.tile([128, NT], F32, tag="mnorm")
    S.activation(mnorm_i[:], lms0[:], AF.Exp, scale=-0.5)

    L = sp.tile([128, NT, N], BF16, tag="L")
    G_.memset(L[:], 0.0)
    LT0 = sp.tile([128, NT, N], BF16, tag="LT")
    G_.memset(LT0[:], 0.0)
    dL0 = sp.tile([128, NT], F32, tag="dL")
    G_.memset(dL0[:], 0.0)

    st = dict(memT=memT, mem_nrm=mem_nrm, mnorm=mnorm_i, L=L, LT=LT0,
              dL=dL0, u_pm=None, prec_pm=None, prec_fl=None,
              pbs=None, rwT=None)

    for t in range(T):
        st = _step(nc, t, st, cp, sp, wp, psB, psM, psS,
                   ident, ones, ones_b, iface, wvR, keysc, neg_er, gr, c1p,
                   cn1, c2, modes1, mbs0, mbs2, nege0_pm, out_sb)

    DMA.dma_start(out_d[:].rearrange("t r w -> w t r"), out_sb[:])


def _step(nc, t, st, cp, sp, wp, psB, psM, psS,
          ident, ones, ones_b, iface, wvR, keysc, neg_er, gr, c1p, cn1, c2,
          modes1, mbs0, mbs2, nege0_pm, out_sb):
    V, S, P, G_, DMA = nc.vector, nc.scalar, nc.tensor, nc.gpsimd, nc.sync
    memT, mem_nrm, mnorm = st["memT"], st["mem_nrm"], st["mnorm"]
    L, LT, dL, u_pm = st["L"], st["LT"], st["dL"], st["u_pm"]
    prec_pm, prec_fl, pbs, rwT = (st["prec_pm"], st["prec_fl"], st["pbs"],
                                  st["rwT"])
    last = (t == T - 1)

    if t == 0:
        na_pm, nsa = nege0_pm, None
    else:
        na_pm, nsa = st["na_pm"], st["nsa"]

    # ---- pbs broadcast (prev-step prec; runs at step start) ----
    if t > 0:
        pb_p = psB.tile([128, N], F32, tag="pb", bufs=2)
        P.matmul(pb_p[:], ones_b[0:1, :], prec_fl[:])
        pbs = wp.tile([128, N], BF16, tag="pbs")
        S.activation(pbs[:], pb_p[:], AF.Copy)

    # ---- write content softmax (PM) ----
    wdots_p = psS.tile([128, NT], F32, tag="s")
    for b in range(NT):
        P.matmul(wdots_p[:, b:b + 1], memT[:, 128 * b:128 * (b + 1)],
                 keysc[:, 4, t:t + 1])
    wlog = wp.tile([128, NT], F32, tag="wlog")
    V.tensor_tensor(wlog[:], wdots_p[:], mnorm[:], op=OP.mult)
    wexp_pm = wp.tile([128, NT], F32, tag="wexp")
    S.activation(wexp_pm[:], wlog[:], AF.Exp)
    wps_p = psS.tile([1, NT], F32, tag="s")
    P.matmul(wps_p[:], ones[:, 0:1], wexp_pm[:])
    wsum = wp.tile([1, 1], F32, tag="wsum")
    V.tensor_reduce(wsum[:], wps_p[:], axis=mybir.AxisListType.X, op=OP.add)
    wrs = wp.tile([1, 1], F32, tag="wrs")
    V.reciprocal(wrs[:], wsum[:])
    cw = wp.tile([1, 1], F32, tag="cw")
    V.tensor_tensor(cw[:], wrs[:], c2[0:1, t:t + 1], op=OP.mult)

    # ---- ww assembly (PM) ----
    cn1b_p = psS.tile([128, 1], F32, tag="s")
    P.matmul(cn1b_p[:], ones[0:1, :], cn1[0:1, t:t + 1])
    cwb_p = psS.tile([128, 1], F32, tag="s")
    P.matmul(cwb_p[:], ones[0:1, :], cw[:])
    wwx = wp.tile([128, NT], F32, tag="wwx")
    V.tensor_scalar(wwx[:], na_pm[:], cn1b_p[:, 0:1], None, op0=OP.mult)
    ww_pm = wp.tile([128, NT], F32, tag="wwpm")
    V.scalar_tensor_tensor(ww_pm[:], wexp_pm[:], cwb_p[:, 0:1], wwx[:],
                           op0=OP.mult, op1=OP.add)
    if t > 0:
        omw_pm = wp.tile([128, NT], F32, tag="omw")
        V.tensor_scalar(omw_pm[:], ww_pm[:], -1.0, 1.0, op0=OP.mult,
                        op1=OP.add)
    if st.get("fin") is not None:
        st["fin"]()
        st["fin"] = None

    ww_tp = psS.tile([1, N], F32, tag="s")
    for c in range(NT):
        P.transpose(ww_tp[0:1, 128 * c:128 * (c + 1)], ww_pm[:, c:c + 1],
                    ident[:])
    ww_fl = wp.tile([1, N], BF16, tag="wwfl")
    S.copy(ww_fl[:], ww_tp[:])

    # ---- wbs broadcast (bf16, for the link ts ops) ----
    if t > 0:
        wb_p = psB.tile([128, N], F32, tag="wb")
        P.matmul(wb_p[:], ones_b[0:1, :], ww_fl[:])
        wbs = wp.tile([128, N], BF16, tag="wbs")
        S.activation(wbs[:], wb_p[:], AF.Copy)

    # ---- memory head ----
    wwb_p = psM.tile([W, N], F32, tag="wwb")
    P.matmul(wwb_p[:], ones_b[0:1, 0:W], ww_fl[:])
    add_p = psM.tile([W, N], F32, tag="add")
    P.matmul(add_p[:], wvR[0:1, t, :], ww_fl[:])
    keep = wp.tile([W, N], F32, tag="keep")
    S.activation(keep[:], wwb_p[:], AF.Copy, scale=neg_er[:, t:t + 1],
                 bias=1.0)
    m1 = wp.tile([W, N], F32, tag="m1")
    G_.tensor_tensor(m1[:], memT[:], keep[:], op=OP.mult)

    # ---- usage update ----
    if last:
        u_pm_n = u_pm
    else:
        u_pm_n = sp.tile([128, NT], F32, tag="u_pm")
        if t == 0:
            V.tensor_copy(u_pm_n[:], ww_pm[:])
        else:
            fgb_p = psS.tile([128, R], F32, tag="s")
            P.matmul(fgb_p[:], ones[0:1, :], gr[0:1, 0:R, t])
            yyT = wp.tile([128, NT, R], F32, tag="yyT")
            V.scalar_tensor_tensor(
                yyT[:], fgb_p[:, None, :].broadcast_to([128, NT, R]), -1.0,
                rwT[:].rearrange("p (c r) -> p c r", r=R),
                op0=OP.mult, op1=OP.mult)
            om = wp.tile([128, NT, R], F32, tag="om")
            V.tensor_scalar(om[:], yyT[:], 1.0, None, op0=OP.add)
            p1u = wp.tile([128, NT], F32, tag="p1u")
            G_.tensor_tensor(p1u[:], om[:, :, 0], om[:, :, 1], op=OP.mult)
            p2u = wp.tile([128, NT], F32, tag="p2u")
            G_.tensor_tensor(p2u[:], om[:, :, 2], om[:, :, 3], op=OP.mult)
            psi = wp.tile([128, NT], F32, tag="psi")
            G_.tensor_tensor(psi[:], p1u[:], p2u[:], op=OP.mult)
            omu = wp.tile([128, NT], F32, tag="omu")
            V.tensor_scalar(omu[:], u_pm[:], -1.0, 1.0, op0=OP.mult,
                            op1=OP.add)
            tn = wp.tile([128, NT], F32, tag="tn")
            V.scalar_tensor_tensor(tn[:], ww_pm[:], 1.0, omu[:],
                                   op0=OP.subtract, op1=OP.mult)
            V.scalar_tensor_tensor(u_pm_n[:], tn[:], 1.0, psi[:],
                                   op0=OP.add, op1=OP.mult)

    # ---- allocation compare inputs (flat u + broadcast; exact fp32) ----
    if not last:
        u_tp = psS.tile([1, N], F32, tag="s")
        for c in range(NT):
            P.transpose(u_tp[0:1, 128 * c:128 * (c + 1)], u_pm_n[:, c:c + 1],
                        ident[:])
        u_fl_n = wp.tile([1, N], F32, tag="ufl")
        S.copy(u_fl_n[:], u_tp[:])
        ub_p = psM.tile([128, N], F32, tag="wwb")
        P.matmul(ub_p[:], ones[0:1, :], u_fl_n[:])
        ubs = wp.tile([128, N], F32, tag="ubs")
        S.copy(ubs[:], ub_p[:])
        ucl = wp.tile([128, NT], F32, tag="ucl")
        V.tensor_scalar(ucl[:], u_pm_n[:], 1e-38, None, op0=OP.max)
        lnu = wp.tile([128, NT], F32, tag="lnu")
        S.activation(lnu[:], ucl[:], AF.Ln)

    # ---- prec update ----
    if not last:
        prec_pm_n = sp.tile([128, NT], F32, tag="prec_pm")
        if t == 0:
            V.tensor_copy(prec_pm_n[:], ww_pm[:])
        else:
            swa = wp.tile([1, 1], F32, tag="swa")
            G_.tensor_tensor(swa[:], nsa[:], cn1[0:1, t:t + 1], op=OP.mult)
            sw = wp.tile([1, 1], F32, tag="sw")
            G_.tensor_tensor(sw[:], swa[:], c2[0:1, t:t + 1], op=OP.add)
            omsw = wp.tile([1, 1], F32, tag="omsw")
            V.tensor_scalar(omsw[:], sw[:], -1.0, 1.0, op0=OP.mult,
                            op1=OP.add)
            omsw_p = psS.tile([128, 1], F32, tag="s")
            P.matmul(omsw_p[:], ones[0:1, :], omsw[:])
            V.scalar_tensor_tensor(prec_pm_n[:], prec_pm[:], omsw_p[:, 0:1],
                                   ww_pm[:], op0=OP.mult, op1=OP.add)
        p_tp = psS.tile([1, N], F32, tag="s")
        for c in range(NT):
            P.transpose(p_tp[0:1, 128 * c:128 * (c + 1)], prec_pm_n[:, c:c + 1],
                        ident[:])
        prec_fl_n = sp.tile([1, N], BF16, tag="prec_fl")
        S.copy(prec_fl_n[:], p_tp[:])
    else:
        prec_pm_n, prec_fl_n = prec_pm, prec_fl

    # ---- mode-scaled read weights + link diagonal tracker ----

    # ---- link loop with interleaved memT_n / Gt compares ----
    comb_eng = [(G_, G_), (G_, V), (G_, V), (G_, V)]
    if t == 0:
        L_n, LT_n = L, LT
        memT_n = sp.tile([W, N], F32, tag="memT")
        V.tensor_tensor(memT_n[:], m1[:], add_p[:], op=OP.add)
        if not last:
            Gt_n = wp.tile([128, NT, N], F32, tag="G", bufs=1)
            for c in range(NT):
                V.tensor_scalar(Gt_n[:, c, :], ubs[:], u_pm_n[:, c:c + 1],
                                None, op0=OP.is_gt)
    else:
        L_n = sp.tile([128, NT, N], BF16, tag="L")
        LT_n = sp.tile([128, NT, N], BF16, tag="LT")
        memT_n = sp.tile([W, N], F32, tag="memT")
        if not last:
            Gt_n = wp.tile([128, NT, N], F32, tag="G", bufs=1)
        for c in range(NT):
            w1 = wp.tile([128, N], BF16, tag=f"w1_{c % 2}")
            V.tensor_scalar(w1[:], wbs[:], omw_pm[:, c:c + 1], None,
                            op0=OP.subtract)
            p1 = wp.tile([128, N], BF16, tag=f"p1_{c % 2}")
            V.tensor_scalar(p1[:], pbs[:], ww_pm[:, c:c + 1], None,
                            op0=OP.mult)
            p1T = wp.tile([128, N], BF16, tag=f"p1T_{c % 2}")
            V.tensor_scalar(p1T[:], wbs[:], prec_pm[:, c:c + 1], None,
                            op0=OP.mult)
            t1 = wp.tile([128, N], BF16, tag=f"t1_{c % 2}")
            G_.tensor_tensor(t1[:], w1[:], L[:, c, :], op=OP.mult)
            t1T = wp.tile([128, N], BF16, tag=f"t1T_{c % 2}")
            G_.tensor_tensor(t1T[:], w1[:], LT[:, c, :], op=OP.mult)
            eL, eLT = comb_eng[c]
            eL.tensor_tensor(L_n[:, c, :], p1[:], t1[:], op=OP.subtract)
            eLT.tensor_tensor(LT_n[:, c, :], p1T[:], t1T[:], op=OP.subtract)
            if c > 0 and not last:
                cc = c - 1
                V.tensor_scalar(Gt_n[:, cc, :], ubs[:],
                                u_pm_n[:, cc:cc + 1], None, op0=OP.is_gt)
        V.tensor_tensor(memT_n[:], m1[:], add_p[:], op=OP.add)
        if not last:
            V.tensor_scalar(Gt_n[:, 3, :], ubs[:], u_pm_n[:, 3:4],
                            None, op0=OP.is_gt)

    if t > 0:
        wp_pm = wp.tile([128, NT], F32, tag="wppm")
        G_.tensor_tensor(wp_pm[:], ww_pm[:], prec_pm[:], op=OP.mult)
        dmul = wp.tile([128, NT], F32, tag="dmul")
        V.tensor_scalar(dmul[:], ww_pm[:], -2.0, 1.0, op0=OP.mult, op1=OP.add)
        dL_n = sp.tile([128, NT], F32, tag="dL")
        V.scalar_tensor_tensor(dL_n[:], dL[:], 1.0, dmul[:],
                               op0=OP.mult, op1=OP.mult)
        G_.tensor_tensor(dL_n[:], dL_n[:], wp_pm[:], op=OP.add)
    else:
        dL_n = dL
    if t > 0:
        rwTm0 = wp.tile([128, NT * R], BF16, tag="rwTm0")
        V.tensor_tensor(rwTm0[:].rearrange("p (c r) -> p c r", r=R),
                        rwT[:].rearrange("p (c r) -> p c r", r=R),
                        mbs0[:, None, :, t].broadcast_to([128, NT, R]),
                        op=OP.mult)
        rwTm2 = wp.tile([128, NT * R], BF16, tag="rwTm2")
        V.tensor_tensor(rwTm2[:].rearrange("p (c r) -> p c r", r=R),
                        rwT[:].rearrange("p (c r) -> p c r", r=R),
                        mbs2[:, None, :, t].broadcast_to([128, NT, R]),
                        op=OP.mult)

    # ---- memory norm chain ----
    mem_nrm_p = psS.tile([128, NT, W], F32, tag="mn", bufs=1)
    for c in range(NT):
        P.transpose(mem_nrm_p[:, c, :], memT_n[:, 128 * c:128 * (c + 1)],
                    ident[0:W, 0:W])
    sqn = wp.tile([128, NT, W], F32, tag="sqn")
    S.activation(sqn[:], mem_nrm_p[:], AF.Square)
    msq = wp.tile([128, NT], F32, tag="msq")
    V.tensor_reduce(msq[:], sqn[:], axis=mybir.AxisListType.X, op=OP.add)
    mem_nrm_n = sp.tile([128, NT, W], F32, tag="mem_nrm")
    S.copy(mem_nrm_n[:], mem_nrm_p[:])
    lms = wp.tile([128, NT], F32, tag="lms")
    S.activation(lms[:], msq[:], AF.Ln)
    mnorm_n = sp.tile([128, NT], F32, tag="mnorm")
    S.activation(mnorm_n[:], lms[:], AF.Exp, scale=-0.5)

    # ---- allocation log-sum ----
    if last:
        na_pm_n, nsa_n = None, None
    else:
        sT_p = psS.tile([128, NT], F32, tag="s")
        for b in range(NT):
            for c in range(NT):
                P.matmul(sT_p[:, b:b + 1], Gt_n[:, c, 128 * b:128 * (b + 1)],
                         lnu[:, c:c + 1], start=(c == 0),
                         stop=(c == NT - 1))
        es_pm = wp.tile([128, NT], F32, tag="espm")
        S.activation(es_pm[:], sT_p[:], AF.Exp)
        na_pm_n = wp.tile([128, NT], F32, tag="napm")
        if t < T - 2:
            nap = wp.tile([128, 1], F32, tag="nap")
            V.scalar_tensor_tensor(na_pm_n[:], u_pm_n[:], 1.0, es_pm[:],
                                   op0=OP.subtract, op1=OP.mult,
                                   accum_out=nap[:])
        else:
            V.scalar_tensor_tensor(na_pm_n[:], u_pm_n[:], 1.0, es_pm[:],
                                   op0=OP.subtract, op1=OP.mult)
        if t < T - 2:
            nsa_p = psS.tile([1, 1], F32, tag="s")
            P.matmul(nsa_p[:], nap[:], ones[:, 0:1])
            nsa_n = wp.tile([1, 1], F32, tag="nsa")
            V.tensor_copy(nsa_n[:], nsa_p[:])
        else:
            nsa_n = None

    # ---- read content (PM) ----
    rdots_p = psS.tile([128, NT * R], F32, tag="s")
    for b in range(NT):
        P.matmul(rdots_p[:, R * b:R * (b + 1)],
                 memT_n[:, 128 * b:128 * (b + 1)], keysc[:, 0:4, t])
    rlog = wp.tile([128, NT, R], F32, tag="rlog")
    V.tensor_tensor(rlog[:],
                    rdots_p[:].rearrange("p (c r) -> p c r", r=R),
                    mnorm_n[:, :, None].broadcast_to([128, NT, R]),
                    op=OP.mult)
    rexp_pm = wp.tile([128, NT * R], F32, tag="rexp")
    S.activation(rexp_pm[:], rlog[:].rearrange("p c r -> p (c r)"), AF.Exp)
    rps_p = psS.tile([1, NT * R], F32, tag="s")
    P.matmul(rps_p[:], ones[:, 0:1], rexp_pm[:])

    def _readmix(t, rps_p=rps_p, rexp_pm=rexp_pm):
        rsum = wp.tile([1, R], F32, tag="rsum")
        V.tensor_reduce(rsum[:], rps_p[:].rearrange("o (c r) -> o r c", r=R),
                        axis=mybir.AxisListType.X, op=OP.add)
        rsr = wp.tile([1, R], F32, tag="rsr")
        V.reciprocal(rsr[:], rsum[:])
        s1c = wp.tile([1, R], F32, tag="s1c")
        V.tensor_tensor(s1c[:], rsr[:], modes1[0:1, :, t], op=OP.mult)
        s1cb_p = psS.tile([128, R], F32, tag="s")
        P.matmul(s1cb_p[:], ones[0:1, :], s1c[:])
        cnt = wp.tile([128, NT, R], F32, tag="cnt")
        V.tensor_tensor(cnt[:], rexp_pm[:].rearrange("p (c r) -> p c r", r=R),
                        s1cb_p[:, None, :].broadcast_to([128, NT, R]),
                        op=OP.mult)
        return cnt

    rwT_n = sp.tile([128, NT * R], F32, tag="rwT")
    if t > 0:
        def _fin(rwT_n=rwT_n, mem_nrm_n=mem_nrm_n, dL_n=dL_n,
                 L_n=L_n, LT_n=LT_n, rwTm0=rwTm0, rwTm2=rwTm2, t=t):
            cnt = _readmix(t)
            s02 = wp.tile([128, NT, R], F32, tag="s02")
            G_.tensor_tensor(s02[:],
                             rwTm0[:].rearrange("p (c r) -> p c r", r=R),
                             rwTm2[:].rearrange("p (c r) -> p c r", r=R),
                             op=OP.add)
            corr = wp.tile([128, NT, R], F32, tag="corr")
            V.tensor_tensor(corr[:], s02[:],
                            dL_n[:, :, None].broadcast_to([128, NT, R]),
                            op=OP.mult)
            cnt2 = wp.tile([128, NT, R], F32, tag="cnt2")
            G_.tensor_tensor(cnt2[:], cnt[:], corr[:], op=OP.subtract)
            rwT_p = psS.tile([128, NT * R], F32, tag="s")
            for b in range(NT):
                blk = slice(128 * b, 128 * (b + 1))
                for c in range(NT):
                    P.matmul(rwT_p[:, R * b:R * (b + 1)], L_n[:, c, blk],
                             rwTm0[:, R * c:R * (c + 1)],
                             start=(c == 0), stop=False)
                for c in range(NT):
                    P.matmul(rwT_p[:, R * b:R * (b + 1)], LT_n[:, c, blk],
                             rwTm2[:, R * c:R * (c + 1)],
                             start=False, stop=(c == NT - 1))
            V.tensor_tensor(rwT_n[:], cnt2[:].rearrange("p c r -> p (c r)"),
                            rwT_p[:], op=OP.add)
            rwd_p = psS.tile([W, R], F32, tag="s")
            for c in range(NT):
                P.matmul(rwd_p[:], mem_nrm_n[:, c, :],
                         rwT_n[:, R * c:R * (c + 1)],
                         start=(c == 0), stop=(c == NT - 1))
            S.copy(out_sb[:, t, :], rwd_p[:])
    else:
        def _fin(rwT_n=rwT_n, mem_nrm_n=mem_nrm_n, t=t):
            cnt = _readmix(t)
            V.tensor_copy(rwT_n[:], cnt[:].rearrange("p c r -> p (c r)"))
            rwd_p = psS.tile([W, R], F32, tag="s")
            for c in range(NT):
                P.matmul(rwd_p[:], mem_nrm_n[:, c, :],
                         rwT_n[:, R * c:R * (c + 1)],
                         start=(c == 0), stop=(c == NT - 1))
            S.copy(out_sb[:, t, :], rwd_p[:])
    if last:
        _fin()
    else:
        st_fin = _fin

    return dict(memT=memT_n, mem_nrm=mem_nrm_n, mnorm=mnorm_n, L=L_n,
                LT=LT_n, dL=dL_n, u_pm=u_pm_n, na_pm=na_pm_n,
                nsa=nsa_n, prec_pm=prec_pm_n, prec_fl=prec_fl_n, pbs=pbs,
                rwT=rwT_n, fin=(None if last else st_fin))


# ---------------------------------------------------------------------------
_NC_CACHE = {}


def _get_nc():
    if "nc" not in _NC_CACHE:
        _NC_CACHE["nc"] = build_nc()
    return _NC_CACHE["nc"]


def _consts():
    ident = np.eye(128, dtype=np.float32)
    return (ident,)


def make_in_maps(controller_output, W_if, b_if, memory0):
    (ident,) = _consts()
    maps = []
    for b in range(B):
        maps.append({
            "co": np.ascontiguousarray(controller_output[b]),
            "wif": np.ascontiguousarray(W_if),
            "bif": np.ascontiguousarray(b_if.reshape(1, IF)),
            "mem0": np.ascontiguousarray(memory0[b]),
            "ident": ident,
        })
    return maps


def kernel(controller_output, W_if, b_if, memory0):
    from concourse.bass_utils import run_bass_kernel_spmd
    controller_output = np.asarray(controller_output, dtype=np.float32)
    W_if = np.asarray(W_if, dtype=np.float32)
    b_if = np.asarray(b_if, dtype=np.float32)
    memory0 = np.asarray(memory0, dtype=np.float32)
    nc = _get_nc()
    maps = make_in_maps(controller_output, W_if, b_if, memory0)
    # Retry once on non-finite output: a stale device (e.g. after an
    # earlier aborted run) can poison cores on the first dispatch.
    for _ in range(2):
        res = run_bass_kernel_spmd(nc, maps, core_ids=list(range(B)))
        out = np.stack([res.results[b]["out"] for b in range(B)], axis=0)
        if np.isfinite(out).all():
            break
    return out


if __name__ == "__main__":
    mode = sys.argv[1] if len(sys.argv) > 1 else "sim"
    sys.path.insert(0, "/root/problem")
    import jax
    with jax.default_device(jax.devices("cpu")[0]):
        import reference
        inputs = {k: np.asarray(v) for k, v in reference.setup_inputs().items()}
        expected = np.asarray(reference.reference(**inputs))

    if mode == "sim":
        from concourse.bass_interp import CoreSim
        nc = build_nc()
        maps = make_in_maps(inputs["controller_output"], inputs["W_if"],
                            inputs["b_if"], inputs["memory0"])
        sim = CoreSim(nc)
        for k, v in maps[0].items():
            sim.tensor(k)[:] = v
        sim.simulate()
        got = sim.tensor("out").copy()
        exp = expected[0]
        err = np.abs(got - exp)
        rel = np.linalg.norm(got - exp) / (np.linalg.norm(exp) + 1e-12)
        print("sim modeled time (ns):", sim.time)
        print("max abs err:", err.max(), " rel err:", rel)
    else:
        got = kernel(**inputs)
        rel = np.linalg.norm(got - expected) / (np.linalg.norm(expected) + 1e-12)
        print("max abs err:", np.abs(got - expected).max(), " rel err:", rel)


# revision 15
# speedup vs baseline: 1.7845x; 1.0174x over previous
"""DNC MemoryAccess kernel for Trainium2 (Bass/Tile), data-parallel over batch.

Shapes (hardcoded): B=8, T=16, C=1024, IFACE=471, N=512, WORD=64, R=4, NW=1.
Each of the 8 cores processes one batch element; all recurrent state stays
SBUF-resident across the T=16 sequential steps.

Design (vs the fp32 predecessor, 326us -> 192us modeled):
- the temporal link matrix L and its transpose LT are held in bf16; the
  elementwise recurrence L' = (1-w_i-w_j)L + w_i p_j runs as fast-mode
  tensor_scalar ops (0.25x DVE cycles in bf16) for w1 = w_j-(1-w_i) and the
  rank-1 terms, with the tensor_tensor multiplies/combines split across
  Pool and DVE,
- the link diagonal is never zeroed in-place: the scalar recurrence
  d' = (1-2w)d + w p is tracked in [128,NT] and its contribution is
  subtracted from the fwd/bwd PE matmul results,
- broadcast matmuls (ww, prec over partitions/words) use bf16 operands
  (1 PE cycle/row vs 4 for fp32); the usage broadcast for the allocation
  sort compare stays exact fp32 so sort ties match the fp32 reference,
- ln(usage) for the allocation cumprod and the memory-norm rsqrt use the
  Activation-table Ln/Exp; get_activation_tables is patched (membership
  only, original set order preserved) so Exp and Ln resolve to the one
  act-func set that contains both, hoisting the 1.3us table load out of
  the step loop,
- emission order is tuned for the per-engine in-order queues: the read
  softmax, rwTm scaling and dL tracker are emitted so the DVE queue never
  head-blocks the ww chain of the next step; the precedence flat vector
  is produced by PE transposes + one Act copy,
- float32r matmuls are NOT used: they fail neuronxcc BIR verification in
  this toolchain (sim accepts them; hardware compile rejects).

Precision: bf16 rounds the link matrices and the write/erase broadcasts
(~1e-3 relative on the output); usage comparisons stay exact fp32 so the
allocation sort matches the reference except for genuine fp32 ties (b=7
carries one, same as the fp32 baseline).
"""
import sys

sys.path.insert(0, "/opt/trn_rl_repo")

import numpy as np

import concourse.bacc as bacc
import concourse.bass as bass
import concourse.mybir as mybir
import concourse.tile as tile

F32 = mybir.dt.float32
F32R = mybir.dt.float32r
BF16 = mybir.dt.bfloat16
I32 = mybir.dt.int32
AF = mybir.ActivationFunctionType
OP = mybir.AluOpType

B, T, C, IF = 8, 16, 1024, 471
N, W, R = 512, 64, 4
NT = N // 128

O_RK, O_RS, O_WK, O_WS = 0, 256, 260, 324
O_ER, O_WV, O_FG, O_AG, O_WG, O_MD = 325, 389, 453, 457, 458, 459


def fr(ap):
    return ap


# Prefer the activation-function set that contains Exp AND Ln (plus
# Copy/Square/Sign), so the per-step Exp/Ln mix resolves to one table and the
# compiler hoists a single LoadActFuncSet out of the step loop instead of
# thrashing 1283ns loads between exp-only and ln-only sets.
_ORIG_GET_ACT_TABLES = None


def _patch_act_tables():
    global _ORIG_GET_ACT_TABLES
    if _ORIG_GET_ACT_TABLES is not None:
        return
    import concourse.hw_specs as hw_specs
    _ORIG_GET_ACT_TABLES = hw_specs.get_activation_tables

    def pinned(arch):
        tabs = dict(_ORIG_GET_ACT_TABLES(arch))
        pref = "natural_log_exp_and_others"
        if pref not in tabs:
            return tabs
        exp_ln = {mybir.ActivationFunctionType.Exp,
                  mybir.ActivationFunctionType.Ln}
        out = {}
        for k, v in tabs.items():
            out[k] = set(v) if k == pref else set(v) - exp_ln
        return out

    bacc.get_activation_tables = pinned


def build_nc():
    _patch_act_tables()
    nc = bacc.Bacc("TRN2", target_bir_lowering=False, debug=False, num_devices=8)

    co_d = nc.declare_dram_parameter("co", [T, C], F32, isOutput=False)
    w_d = nc.declare_dram_parameter("wif", [C, IF], F32, isOutput=False)
    b_d = nc.declare_dram_parameter("bif", [1, IF], F32, isOutput=False)
    m0_d = nc.declare_dram_parameter("mem0", [N, W], F32, isOutput=False)
    ident_d = nc.declare_dram_parameter("ident", [128, 128], F32, isOutput=False)
    out_d = nc.declare_dram_parameter("out", [T, R, W], F32, isOutput=True)

    with tile.TileContext(nc) as tc:
        with (
            nc.allow_low_precision(reason="bf16 link + f32r broadcasts stay"
                                   " within the 2e-2 gate"),
            tc.tile_pool(name="const", bufs=1) as cp,
            tc.tile_pool(name="state", bufs=2) as sp,
            tc.tile_pool(name="work", bufs=2) as wp,
            tc.tile_pool(name="psBig", bufs=1, space="PSUM") as psB,
            tc.tile_pool(name="psMem", bufs=1, space="PSUM") as psM,
            tc.tile_pool(name="psS", bufs=2, space="PSUM") as psS,
        ):
            _build_body(nc, tc, cp, sp, wp, psB, psM, psS,
                        co_d, w_d, b_d, m0_d, ident_d, out_d)
    nc.compile()
    return nc


def _build_body(nc, tc, cp, sp, wp, psB, psM, psS,
                co_d, w_d, b_d, m0_d, ident_d, out_d):
    V, S, P, G_, DMA = nc.vector, nc.scalar, nc.tensor, nc.gpsimd, nc.sync

    # ---------------- constants ----------------
    ident = cp.tile([128, 128], F32)
    DMA.dma_start(ident[:], ident_d[:])
    ones = cp.tile([128, 128], F32)
    G_.memset(ones[:], 1.0)
    ones_b = cp.tile([1, 128], BF16)
    G_.memset(ones_b[:], 1.0)
    ident_b = cp.tile([128, 128], BF16)
    V.tensor_copy(ident_b[:], ident[:])

    # persistent per-t tables
    iface = cp.tile([T, IF], F32)          # raw iface rows
    wvR = cp.tile([1, T, W], BF16)         # write vectors, partition-0 rows
    keysc = cp.tile([W, 5, T], F32)        # scaled keys: r=0..3 read, 4 write
    neg_er = cp.tile([W, T], F32)
    gr = cp.tile([1, 6, T], F32)           # sigmoids: fg x4, ag, wg
    c1p = cp.tile([1, T], F32)
    cn1 = cp.tile([1, T], F32)
    c2 = cp.tile([1, T], F32)
    modes1 = cp.tile([1, R, T], F32)       # content-mode row per t
    mbs0 = cp.tile([128, R, T], F32)
    mbs2 = cp.tile([128, R, T], F32)
    nege0_pm = cp.tile([128, NT], F32)
    G_.memset(nege0_pm[:], 0.0)
    G_.memset(nege0_pm[0:1, 0:1], -1.0)
    out_sb = cp.tile([W, T, R], F32)

    # ---------------- prologue ----------------
    with tc.tile_pool(name="prolog", bufs=1) as pp:
        co_sb = pp.tile([T, C], F32)
        DMA.dma_start(co_sb[:], co_d[:])
        bif_sb = pp.tile([1, IF], F32)
        DMA.dma_start(bif_sb[:], b_d[:])
        w_sb = pp.tile([128, 8, IF], F32)
        for k in range(8):
            # split the 1.9MB load across two hwdge queues
            eng = DMA if k % 2 == 0 else nc.scalar
            eng.dma_start(w_sb[:, k, :], w_d[128 * k:128 * (k + 1), :])

        coT_p = psB.tile([128, 8, T], F32, tag="wb")
        for k in range(8):
            P.transpose(coT_p[:, k, :], co_sb[:, 128 * k:128 * (k + 1)],
                        ident[0:T, 0:T])
        coT = pp.tile([128, 8, T], F32)
        V.tensor_copy(coT[:], coT_p[:])

        if_p = psB.tile([T, IF], F32, tag="pb", bufs=2)
        for k in range(8):
            P.matmul(if_p[:], coT[:, k, :], w_sb[:, k, :],
                     start=(k == 0), stop=False)
        P.matmul(if_p[:], ones[0:1, 0:T], bif_sb[:],
                 start=False, stop=True)
        V.tensor_copy(iface[:], if_p[:])

        # keys [64, 5, T]: read r=0..3, write at 4
        keys_p = psB.tile([W, 5, T], F32, tag="pb", bufs=2)
        for r in range(R):
            P.transpose(keys_p[:, r, :], iface[:, O_RK + W * r:O_RK + W * (r + 1)],
                        ident[0:T, 0:T])
        P.transpose(keys_p[:, 4, :], iface[:, O_WK:O_WK + W], ident[0:T, 0:T])
        keys = pp.tile([W, 5, T], F32)
        V.tensor_copy(keys[:], keys_p[:])

        # write vectors as partition-0 rows via selector matmuls, two copies
        for h in range(2):
            wv_p = psB.tile([1, 8, W], F32, tag="pb", bufs=2, name=f"wvp{h}")
            for j in range(8):
                tt_ = 8 * h + j
                P.matmul(wv_p[0:1, j, :], ident[0:T, tt_:tt_ + 1],
                         iface[:, O_WV:O_WV + W])
            V.tensor_copy(wvR[0:1, 8 * h:8 * (h + 1), :].rearrange(
                "o t w -> o (t w)"),
                wv_p[:].rearrange("o t w -> o (t w)"))

        # erase sigmoid -> neg_er
        er_p = psS.tile([W, T], F32, tag="s")
        P.transpose(er_p[:], iface[:, O_ER:O_ER + W], ident[0:T, 0:T])
        ee = pp.tile([W, T], F32)
        S.activation(ee[:], er_p[:], AF.Exp, scale=-1.0)
        ew = pp.tile([W, T], F32)
        V.tensor_scalar(ew[:], ee[:], 1.0, None, op0=OP.add)
        er_r = pp.tile([W, T], F32)
        V.reciprocal(er_r[:], ew[:])
        V.tensor_scalar(neg_er[:], er_r[:], -1.0, None, op0=OP.mult)

        # strengths softplus: [1, 5, T] (rs x4, ws)
        sts_p = psS.tile([1, 5, T], F32, tag="s")
        for r in range(R):
            P.transpose(sts_p[0:1, r, :], iface[:, O_RS + r:O_RS + r + 1],
                        ident[0:T, 0:T])
        P.transpose(sts_p[0:1, 4, :], iface[:, O_WS:O_WS + 1], ident[0:T, 0:T])
        st_e = pp.tile([1, 5 * T], F32)
        S.activation(st_e[:], sts_p[:].rearrange("o f t -> o (f t)"), AF.Exp)
        st_w = pp.tile([1, 5 * T], F32)
        V.tensor_scalar(st_w[:], st_e[:], 1.0, None, op0=OP.add)
        st_sp = pp.tile([1, 5 * T], F32)
        S.activation(st_sp[:], st_w[:], AF.Ln)

        # key norms: rsqrt(sum keys^2) = exp(-0.5 ln)
        sqk = pp.tile([W, 5 * T], F32)
        S.activation(sqk[:], keys[:].rearrange("w f t -> w (f t)"), AF.Square)
        k2_p = psM.tile([1, 5 * T], F32, tag="wwb")
        P.matmul(k2_p[:], ones[0:W, 0:1], sqk[:])
        lk2 = pp.tile([1, 5 * T], F32)
        S.activation(lk2[:], k2_p[:], AF.Ln)
        kr = pp.tile([1, 5 * T], F32)
        S.activation(kr[:], lk2[:], AF.Exp, scale=-0.5)
        beta = pp.tile([1, 5 * T], F32)
        V.tensor_tensor(beta[:], st_sp[:], kr[:], op=OP.mult)
        kb_p = psM.tile([W, 5 * T], F32, tag="add")
        P.matmul(kb_p[:], ones[0:1, 0:W], beta[:])
        V.tensor_tensor(keysc[:].rearrange("w f t -> w (f t)"),
                        keys[:].rearrange("w f t -> w (f t)"), kb_p[:],
                        op=OP.mult)

        # gates: fg x4, ag, wg sigmoids
        gats_p = psS.tile([1, 6, T], F32, tag="s")
        for r in range(R):
            P.transpose(gats_p[0:1, r, :], iface[:, O_FG + r:O_FG + r + 1],
                        ident[0:T, 0:T])
        P.transpose(gats_p[0:1, 4, :], iface[:, O_AG:O_AG + 1], ident[0:T, 0:T])
        P.transpose(gats_p[0:1, 5, :], iface[:, O_WG:O_WG + 1], ident[0:T, 0:T])
        g_e = pp.tile([1, 6 * T], F32)
        S.activation(g_e[:], gats_p[:].rearrange("o g t -> o (g t)"), AF.Exp,
                     scale=-1.0)
        g_w = pp.tile([1, 6 * T], F32)
        V.tensor_scalar(g_w[:], g_e[:], 1.0, None, op0=OP.add)
        V.reciprocal(gr[:].rearrange("o g t -> o (g t)"), g_w[:])
        ag_t = gr[0:1, 4, :]
        wg_t = gr[0:1, 5, :]
        V.tensor_tensor(c1p[:], ag_t, wg_t, op=OP.mult)
        V.tensor_scalar(cn1[:], c1p[:], -1.0, None, op0=OP.mult)
        V.tensor_tensor(c2[:], wg_t, c1p[:], op=OP.subtract)

        # modes softmax -> rows per t
        me = pp.tile([T, 12], F32)
        S.activation(me[:], iface[:, O_MD:O_MD + 12], AF.Exp)
        me3 = me[:].rearrange("t (r m) -> t r m", m=3)
        msum = pp.tile([T, R], F32)
        V.tensor_tensor(msum[:], me3[:, :, 0], me3[:, :, 1], op=OP.add)
        V.tensor_tensor(msum[:], msum[:], me3[:, :, 2], op=OP.add)
        mrcp = pp.tile([T, R], F32)
        V.reciprocal(mrcp[:], msum[:])
        mn = pp.tile([T, 12], F32)
        mn3 = mn[:].rearrange("t (m r) -> t m r", r=R)
        me3b = me[:].rearrange("t (r m) -> t m r", m=3)
        for m in range(3):
            V.tensor_tensor(mn3[:, m, :], me3b[:, m, :], mrcp[:], op=OP.mult)
        # three m-blocks at base partition 0: modes0/1/2 [4, T]
        mblk_p = psS.tile([R, 3, T], F32, tag="s")
        for m in range(3):
            P.transpose(mblk_p[:, m, :], mn[:, 4 * m:4 * (m + 1)],
                        ident[0:T, 0:T])
        mblk = pp.tile([R, 3, T], F32)
        V.tensor_copy(mblk[:], mblk_p[:])
        m1sel_p = psS.tile([1, R, T], F32, tag="s")
        for r in range(R):
            P.matmul(m1sel_p[0:1, r, :], ident[0:R, r:r + 1], mblk[:, 1, :])
        V.tensor_copy(modes1[:].rearrange("o r t -> o (r t)"),
                      m1sel_p[:].rearrange("o r t -> o (r t)"))
        # flatten rows r of m-block 0/2 onto partition 0 via selector matmuls
        mrows_p = psS.tile([1, 2, R, T], F32, tag="s")
        for r in range(R):
            P.matmul(mrows_p[0:1, 0, r, :], ident[0:R, r:r + 1], mblk[:, 0, :])
            P.matmul(mrows_p[0:1, 1, r, :], ident[0:R, r:r + 1], mblk[:, 2, :])
        mrows = pp.tile([1, 2, R, T], F32)
        V.tensor_copy(mrows[:].rearrange("o a r t -> o (a r t)"),
                      mrows_p[:].rearrange("o a r t -> o (a r t)"))
        mb0_p = psB.tile([128, R * T], F32, tag="wb")
        P.matmul(mb0_p[:], ones[0:1, :], mrows[0:1, 0, :, :])
        V.tensor_copy(mbs0[:].rearrange("p r t -> p (r t)"), mb0_p[:])
        mb2_p = psB.tile([128, R * T], F32, tag="pb", bufs=2)
        P.matmul(mb2_p[:], ones[0:1, :], mrows[0:1, 1, :, :])
        V.tensor_copy(mbs2[:].rearrange("p r t -> p (r t)"), mb2_p[:])

    # ---------------- initial state ----------------
    mem_nrm = sp.tile([128, NT, W], F32, tag="mem_nrm")
    for c in range(NT):
        DMA.dma_start(mem_nrm[:, c, :],
                      m0_d[128 * c:128 * (c + 1), :])
    memT_p = psB.tile([W, N], F32, tag="wb")
    for c in range(NT):
        P.transpose(memT_p[:, 128 * c:128 * (c + 1)],
                    mem_nrm[:, c, :], ident[:])
    memT = sp.tile([W, N], F32, tag="memT")
    V.tensor_copy(memT[:], memT_p[:])

    # initial norm: PM-layout sqn -> msq -> Ln/Exp
    sqn0 = wp.tile([128, NT, W], F32, tag="sqn")
    G_.tensor_tensor(sqn0[:], mem_nrm[:], mem_nrm[:], op=OP.mult)
    msq0 = wp.tile([128, NT], F32, tag="msq")
    V.tensor_reduce(msq0[:], sqn0[:], axis=mybir.AxisListType.X, op=OP.add)
    lms0 = wp.tile([128, NT], F32, tag="lms")
    S.activation(lms0[:], msq0[:], AF.Ln)
    mnorm_i = sp.tile([128, NT], F32, tag="mnorm")
    S.activation(mnorm_i[:], lms0[:], AF.Exp, scale=-0.5)

    L = sp.tile([128, NT, N], BF16, tag="L")
    G_.memset(L[:], 0.0)
    LT0 = sp.tile([128, NT, N], BF16, tag="LT")
    G_.memset(LT0[:], 0.0)
    dL0 = sp.tile([128, NT], F32, tag="dL")
    G_.memset(dL0[:], 0.0)

    st = dict(memT=memT, mem_nrm=mem_nrm, mnorm=mnorm_i, L=L, LT=LT0,
              dL=dL0, u_pm=None, prec_pm=None, prec_fl=None,
              pbs=None, rwT=None)

    for t in range(T):
        st = _step(nc, t, st, cp, sp, wp, psB, psM, psS,
                   ident, ident_b, ones, ones_b, iface, wvR, keysc, neg_er,
                   gr, c1p, cn1, c2, modes1, mbs0, mbs2, nege0_pm, out_sb)

    DMA.dma_start(out_d[:].rearrange("t r w -> w t r"), out_sb[:])


def _step(nc, t, st, cp, sp, wp, psB, psM, psS,
          ident, ident_b, ones, ones_b, iface, wvR, keysc, neg_er, gr, c1p,
          cn1, c2, modes1, mbs0, mbs2, nege0_pm, out_sb):
    V, S, P, G_, DMA = nc.vector, nc.scalar, nc.tensor, nc.gpsimd, nc.sync
    memT, mem_nrm, mnorm = st["memT"], st["mem_nrm"], st["mnorm"]
    L, LT, dL, u_pm = st["L"], st["LT"], st["dL"], st["u_pm"]
    prec_pm, prec_fl, pbs, rwT = (st["prec_pm"], st["prec_fl"], st["pbs"],
                                  st["rwT"])
    last = (t == T - 1)

    if t == 0:
        na_pm, nsa = nege0_pm, None
    else:
        na_pm, nsa = st["na_pm"], st["nsa"]

    # ---- pbs broadcast (prev-step prec; runs at step start) ----
    if t > 0:
        pb_p = psB.tile([128, N], F32, tag="pb", bufs=2)
        P.matmul(pb_p[:], ones_b[0:1, :], prec_fl[:])
        pbs = wp.tile([128, N], BF16, tag="pbs")
        S.activation(pbs[:], pb_p[:], AF.Copy)

    # ---- write content softmax (PM) ----
    wdots_p = psS.tile([128, NT], F32, tag="s")
    for b in range(NT):
        P.matmul(wdots_p[:, b:b + 1], memT[:, 128 * b:128 * (b + 1)],
                 keysc[:, 4, t:t + 1])
    wlog = wp.tile([128, NT], F32, tag="wlog")
    V.tensor_tensor(wlog[:], wdots_p[:], mnorm[:], op=OP.mult)
    wexp_pm = wp.tile([128, NT], F32, tag="wexp")
    S.activation(wexp_pm[:], wlog[:], AF.Exp)
    wps_p = psS.tile([1, NT], F32, tag="s")
    P.matmul(wps_p[:], ones[:, 0:1], wexp_pm[:])
    wsum = wp.tile([1, 1], F32, tag="wsum")
    V.tensor_reduce(wsum[:], wps_p[:], axis=mybir.AxisListType.X, op=OP.add)
    wrs = wp.tile([1, 1], F32, tag="wrs")
    V.reciprocal(wrs[:], wsum[:])
    cw = wp.tile([1, 1], F32, tag="cw")
    V.tensor_tensor(cw[:], wrs[:], c2[0:1, t:t + 1], op=OP.mult)

    # ---- ww assembly (PM) ----
    cn1b_p = psS.tile([128, 1], F32, tag="s")
    P.matmul(cn1b_p[:], ones[0:1, :], cn1[0:1, t:t + 1])
    cwb_p = psS.tile([128, 1], F32, tag="s")
    P.matmul(cwb_p[:], ones[0:1, :], cw[:])
    wwx = wp.tile([128, NT], F32, tag="wwx")
    V.tensor_scalar(wwx[:], na_pm[:], cn1b_p[:, 0:1], None, op0=OP.mult)
    ww_pm = wp.tile([128, NT], F32, tag="wwpm")
    V.scalar_tensor_tensor(ww_pm[:], wexp_pm[:], cwb_p[:, 0:1], wwx[:],
                           op0=OP.mult, op1=OP.add)
    if t > 0:
        omw_pm = wp.tile([128, NT], F32, tag="omw")
        V.tensor_scalar(omw_pm[:], ww_pm[:], -1.0, 1.0, op0=OP.mult,
                        op1=OP.add)
    if st.get("fin") is not None:
        st["fin"]()
        st["fin"] = None

    wwpm_b = wp.tile([128, NT], BF16, tag="wwpmb")
    V.tensor_copy(wwpm_b[:], ww_pm[:])
    ww_tp = psS.tile([1, N], BF16, tag="s")
    for c in range(NT):
        P.transpose(ww_tp[0:1, 128 * c:128 * (c + 1)], wwpm_b[:, c:c + 1],
                    ident_b[:])
    ww_fl = wp.tile([1, N], BF16, tag="wwfl")
    S.copy(ww_fl[:], ww_tp[:])

    # ---- wbs broadcast (bf16, for the link ts ops) ----
    if t > 0:
        wb_p = psB.tile([128, N], F32, tag="wb")
        P.matmul(wb_p[:], ones_b[0:1, :], ww_fl[:])
        wbs = wp.tile([128, N], BF16, tag="wbs")
        S.activation(wbs[:], wb_p[:], AF.Copy)

    # ---- memory head ----
    wwb_p = psM.tile([W, N], F32, tag="wwb")
    P.matmul(wwb_p[:], ones_b[0:1, 0:W], ww_fl[:])
    add_p = psM.tile([W, N], F32, tag="add")
    P.matmul(add_p[:], wvR[0:1, t, :], ww_fl[:])
    keep = wp.tile([W, N], F32, tag="keep")
    S.activation(keep[:], wwb_p[:], AF.Copy, scale=neg_er[:, t:t + 1],
                 bias=1.0)
    m1 = wp.tile([W, N], F32, tag="m1")
    G_.tensor_tensor(m1[:], memT[:], keep[:], op=OP.mult)

    # ---- usage update ----
    if last:
        u_pm_n = u_pm
    else:
        u_pm_n = sp.tile([128, NT], F32, tag="u_pm")
        if t == 0:
            V.tensor_copy(u_pm_n[:], ww_pm[:])
        else:
            fgb_p = psS.tile([128, R], F32, tag="s")
            P.matmul(fgb_p[:], ones[0:1, :], gr[0:1, 0:R, t])
            yyT = wp.tile([128, NT, R], F32, tag="yyT")
            V.scalar_tensor_tensor(
                yyT[:], fgb_p[:, None, :].broadcast_to([128, NT, R]), -1.0,
                rwT[:].rearrange("p (c r) -> p c r", r=R),
                op0=OP.mult, op1=OP.mult)
            om = wp.tile([128, NT, R], F32, tag="om")
            V.tensor_scalar(om[:], yyT[:], 1.0, None, op0=OP.add)
            p1u = wp.tile([128, NT], F32, tag="p1u")
            G_.tensor_tensor(p1u[:], om[:, :, 0], om[:, :, 1], op=OP.mult)
            p2u = wp.tile([128, NT], F32, tag="p2u")
            G_.tensor_tensor(p2u[:], om[:, :, 2], om[:, :, 3], op=OP.mult)
            psi = wp.tile([128, NT], F32, tag="psi")
            G_.tensor_tensor(psi[:], p1u[:], p2u[:], op=OP.mult)
            omu = wp.tile([128, NT], F32, tag="omu")
            V.tensor_scalar(omu[:], u_pm[:], -1.0, 1.0, op0=OP.mult,
                            op1=OP.add)
            tn = wp.tile([128, NT], F32, tag="tn")
            V.scalar_tensor_tensor(tn[:], ww_pm[:], 1.0, omu[:],
                                   op0=OP.subtract, op1=OP.mult)
            V.scalar_tensor_tensor(u_pm_n[:], tn[:], 1.0, psi[:],
                                   op0=OP.add, op1=OP.mult)

    # ---- allocation compare inputs (flat u + broadcast; exact fp32) ----
    if not last:
        u_tp = psS.tile([1, N], F32, tag="s")
        for c in range(NT):
            P.transpose(u_tp[0:1, 128 * c:128 * (c + 1)], u_pm_n[:, c:c + 1],
                        ident[:])
        u_fl_n = wp.tile([1, N], F32, tag="ufl")
        S.copy(u_fl_n[:], u_tp[:])
        ub_p = psM.tile([128, N], F32, tag="wwb")
        P.matmul(ub_p[:], ones[0:1, :], u_fl_n[:])
        ubs = wp.tile([128, N], F32, tag="ubs")
        S.copy(ubs[:], ub_p[:])
        ucl = wp.tile([128, NT], F32, tag="ucl")
        V.tensor_scalar(ucl[:], u_pm_n[:], 1e-38, None, op0=OP.max)
        lnu = wp.tile([128, NT], F32, tag="lnu")
        S.activation(lnu[:], ucl[:], AF.Ln)

    # ---- prec update ----
    if not last:
        prec_pm_n = sp.tile([128, NT], F32, tag="prec_pm")
        if t == 0:
            V.tensor_copy(prec_pm_n[:], ww_pm[:])
        else:
            swa = wp.tile([1, 1], F32, tag="swa")
            G_.tensor_tensor(swa[:], nsa[:], cn1[0:1, t:t + 1], op=OP.mult)
            sw = wp.tile([1, 1], F32, tag="sw")
            G_.tensor_tensor(sw[:], swa[:], c2[0:1, t:t + 1], op=OP.add)
            omsw = wp.tile([1, 1], F32, tag="omsw")
            V.tensor_scalar(omsw[:], sw[:], -1.0, 1.0, op0=OP.mult,
                            op1=OP.add)
            omsw_p = psS.tile([128, 1], F32, tag="s")
            P.matmul(omsw_p[:], ones[0:1, :], omsw[:])
            V.scalar_tensor_tensor(prec_pm_n[:], prec_pm[:], omsw_p[:, 0:1],
                                   ww_pm[:], op0=OP.mult, op1=OP.add)
        p_tp = psS.tile([1, N], F32, tag="s")
        for c in range(NT):
            P.transpose(p_tp[0:1, 128 * c:128 * (c + 1)], prec_pm_n[:, c:c + 1],
                        ident[:])
        prec_fl_n = sp.tile([1, N], BF16, tag="prec_fl")
        S.copy(prec_fl_n[:], p_tp[:])
    else:
        prec_pm_n, prec_fl_n = prec_pm, prec_fl

    # ---- mode-scaled read weights + link diagonal tracker ----

    # ---- link loop with interleaved memT_n / Gt compares ----
    comb_eng = [(G_, G_), (G_, V), (G_, V), (G_, V)]
    if t == 0:
        L_n, LT_n = L, LT
        memT_n = sp.tile([W, N], F32, tag="memT")
        V.tensor_tensor(memT_n[:], m1[:], add_p[:], op=OP.add)
        if not last:
            Gt_n = wp.tile([128, NT, N], F32, tag="G", bufs=1)
            for c in range(NT):
                V.tensor_scalar(Gt_n[:, c, :], ubs[:], u_pm_n[:, c:c + 1],
                                None, op0=OP.is_gt)
    else:
        L_n = sp.tile([128, NT, N], BF16, tag="L")
        LT_n = sp.tile([128, NT, N], BF16, tag="LT")
        memT_n = sp.tile([W, N], F32, tag="memT")
        if not last:
            Gt_n = wp.tile([128, NT, N], F32, tag="G", bufs=1)
        for c in range(NT):
            w1 = wp.tile([128, N], BF16, tag=f"w1_{c % 2}")
            V.tensor_scalar(w1[:], wbs[:], omw_pm[:, c:c + 1], None,
                            op0=OP.subtract)
            p1 = wp.tile([128, N], BF16, tag=f"p1_{c % 2}")
            V.tensor_scalar(p1[:], pbs[:], ww_pm[:, c:c + 1], None,
                            op0=OP.mult)
            p1T = wp.tile([128, N], BF16, tag=f"p1T_{c % 2}")
            V.tensor_scalar(p1T[:], wbs[:], prec_pm[:, c:c + 1], None,
                            op0=OP.mult)
            t1 = wp.tile([128, N], BF16, tag=f"t1_{c % 2}")
            G_.tensor_tensor(t1[:], w1[:], L[:, c, :], op=OP.mult)
            t1T = wp.tile([128, N], BF16, tag=f"t1T_{c % 2}")
            G_.tensor_tensor(t1T[:], w1[:], LT[:, c, :], op=OP.mult)
            eL, eLT = comb_eng[c]
            eL.tensor_tensor(L_n[:, c, :], p1[:], t1[:], op=OP.subtract)
            eLT.tensor_tensor(LT_n[:, c, :], p1T[:], t1T[:], op=OP.subtract)
            if c > 0 and not last:
                cc = c - 1
                V.tensor_scalar(Gt_n[:, cc, :], ubs[:],
                                u_pm_n[:, cc:cc + 1], None, op0=OP.is_gt)
        V.tensor_tensor(memT_n[:], m1[:], add_p[:], op=OP.add)
        if not last:
            V.tensor_scalar(Gt_n[:, 3, :], ubs[:], u_pm_n[:, 3:4],
                            None, op0=OP.is_gt)

    if t > 0:
        wp_pm = wp.tile([128, NT], F32, tag="wppm")
        G_.tensor_tensor(wp_pm[:], ww_pm[:], prec_pm[:], op=OP.mult)
        dmul = wp.tile([128, NT], F32, tag="dmul")
        V.tensor_scalar(dmul[:], ww_pm[:], -2.0, 1.0, op0=OP.mult, op1=OP.add)
        dL_n = sp.tile([128, NT], F32, tag="dL")
        V.scalar_tensor_tensor(dL_n[:], dL[:], 1.0, dmul[:],
                               op0=OP.mult, op1=OP.mult)
        G_.tensor_tensor(dL_n[:], dL_n[:], wp_pm[:], op=OP.add)
    else:
        dL_n = dL
    if t > 0:
        rwTm0 = wp.tile([128, NT * R], BF16, tag="rwTm0")
        V.tensor_tensor(rwTm0[:].rearrange("p (c r) -> p c r", r=R),
                        rwT[:].rearrange("p (c r) -> p c r", r=R),
                        mbs0[:, None, :, t].broadcast_to([128, NT, R]),
                        op=OP.mult)
        rwTm2 = wp.tile([128, NT * R], BF16, tag="rwTm2")
        V.tensor_tensor(rwTm2[:].rearrange("p (c r) -> p c r", r=R),
                        rwT[:].rearrange("p (c r) -> p c r", r=R),
                        mbs2[:, None, :, t].broadcast_to([128, NT, R]),
                        op=OP.mult)

    # ---- memory norm chain ----
    mem_nrm_p = psS.tile([128, NT, W], F32, tag="mn", bufs=1)
    for c in range(NT):
        P.transpose(mem_nrm_p[:, c, :], memT_n[:, 128 * c:128 * (c + 1)],
                    ident[0:W, 0:W])
    sqn = wp.tile([128, NT, W], F32, tag="sqn")
    S.activation(sqn[:], mem_nrm_p[:], AF.Square)
    msq = wp.tile([128, NT], F32, tag="msq")
    V.tensor_reduce(msq[:], sqn[:], axis=mybir.AxisListType.X, op=OP.add)
    mem_nrm_n = sp.tile([128, NT, W], F32, tag="mem_nrm")
    S.copy(mem_nrm_n[:], mem_nrm_p[:])
    lms = wp.tile([128, NT], F32, tag="lms")
    S.activation(lms[:], msq[:], AF.Ln)
    mnorm_n = sp.tile([128, NT], F32, tag="mnorm")
    S.activation(mnorm_n[:], lms[:], AF.Exp, scale=-0.5)

    # ---- allocation log-sum ----
    if last:
        na_pm_n, nsa_n = None, None
    else:
        sT_p = psS.tile([128, NT], F32, tag="s")
        for b in range(NT):
            for c in range(NT):
                P.matmul(sT_p[:, b:b + 1], Gt_n[:, c, 128 * b:128 * (b + 1)],
                         lnu[:, c:c + 1], start=(c == 0),
                         stop=(c == NT - 1))
        es_pm = wp.tile([128, NT], F32, tag="espm")
        S.activation(es_pm[:], sT_p[:], AF.Exp)
        na_pm_n = wp.tile([128, NT], F32, tag="napm")
        if t < T - 2:
            nap = wp.tile([128, 1], F32, tag="nap")
            V.scalar_tensor_tensor(na_pm_n[:], u_pm_n[:], 1.0, es_pm[:],
                                   op0=OP.subtract, op1=OP.mult,
                                   accum_out=nap[:])
        else:
            V.scalar_tensor_tensor(na_pm_n[:], u_pm_n[:], 1.0, es_pm[:],
                                   op0=OP.subtract, op1=OP.mult)
        if t < T - 2:
            nsa_p = psS.tile([1, 1], F32, tag="s")
            P.matmul(nsa_p[:], nap[:], ones[:, 0:1])
            nsa_n = wp.tile([1, 1], F32, tag="nsa")
            V.tensor_copy(nsa_n[:], nsa_p[:])
        else:
            nsa_n = None

    # ---- read content (PM) ----
    rdots_p = psS.tile([128, NT * R], F32, tag="s")
    for b in range(NT):
        P.matmul(rdots_p[:, R * b:R * (b + 1)],
                 memT_n[:, 128 * b:128 * (b + 1)], keysc[:, 0:4, t])
    rlog = wp.tile([128, NT, R], F32, tag="rlog")
    V.tensor_tensor(rlog[:],
                    rdots_p[:].rearrange("p (c r) -> p c r", r=R),
                    mnorm_n[:, :, None].broadcast_to([128, NT, R]),
                    op=OP.mult)
    rexp_pm = wp.tile([128, NT * R], F32, tag="rexp")
    S.activation(rexp_pm[:], rlog[:].rearrange("p c r -> p (c r)"), AF.Exp)
    rps_p = psS.tile([1, NT * R], F32, tag="s")
    P.matmul(rps_p[:], ones[:, 0:1], rexp_pm[:])

    def _readmix(t, rps_p=rps_p, rexp_pm=rexp_pm):
        rsum = wp.tile([1, R], F32, tag="rsum")
        V.tensor_reduce(rsum[:], rps_p[:].rearrange("o (c r) -> o r c", r=R),
                        axis=mybir.AxisListType.X, op=OP.add)
        rsr = wp.tile([1, R], F32, tag="rsr")
        V.reciprocal(rsr[:], rsum[:])
        s1c = wp.tile([1, R], F32, tag="s1c")
        V.tensor_tensor(s1c[:], rsr[:], modes1[0:1, :, t], op=OP.mult)
        s1cb_p = psS.tile([128, R], F32, tag="s")
        P.matmul(s1cb_p[:], ones[0:1, :], s1c[:])
        cnt = wp.tile([128, NT, R], F32, tag="cnt")
        V.tensor_tensor(cnt[:], rexp_pm[:].rearrange("p (c r) -> p c r", r=R),
                        s1cb_p[:, None, :].broadcast_to([128, NT, R]),
                        op=OP.mult)
        return cnt

    rwT_n = sp.tile([128, NT * R], F32, tag="rwT")
    if t > 0:
        def _fin(rwT_n=rwT_n, mem_nrm_n=mem_nrm_n, dL_n=dL_n,
                 L_n=L_n, LT_n=LT_n, rwTm0=rwTm0, rwTm2=rwTm2, t=t):
            cnt = _readmix(t)
            s02 = wp.tile([128, NT, R], F32, tag="s02")
            G_.tensor_tensor(s02[:],
                             rwTm0[:].rearrange("p (c r) -> p c r", r=R),
                             rwTm2[:].rearrange("p (c r) -> p c r", r=R),
                             op=OP.add)
            corr = wp.tile([128, NT, R], F32, tag="corr")
            V.tensor_tensor(corr[:], s02[:],
                            dL_n[:, :, None].broadcast_to([128, NT, R]),
                            op=OP.mult)
            cnt2 = wp.tile([128, NT, R], F32, tag="cnt2")
            G_.tensor_tensor(cnt2[:], cnt[:], corr[:], op=OP.subtract)
            rwT_p = psS.tile([128, NT * R], F32, tag="s")
            for b in range(NT):
                blk = slice(128 * b, 128 * (b + 1))
                for c in range(NT):
                    P.matmul(rwT_p[:, R * b:R * (b + 1)], L_n[:, c, blk],
                             rwTm0[:, R * c:R * (c + 1)],
                             start=(c == 0), stop=False)
                for c in range(NT):
                    P.matmul(rwT_p[:, R * b:R * (b + 1)], LT_n[:, c, blk],
                             rwTm2[:, R * c:R * (c + 1)],
                             start=False, stop=(c == NT - 1))
            V.tensor_tensor(rwT_n[:], cnt2[:].rearrange("p c r -> p (c r)"),
                            rwT_p[:], op=OP.add)
            rwd_p = psS.tile([W, R], F32, tag="s")
            for c in range(NT):
                P.matmul(rwd_p[:], mem_nrm_n[:, c, :],
                         rwT_n[:, R * c:R * (c + 1)],
                         start=(c == 0), stop=(c == NT - 1))
            S.copy(out_sb[:, t, :], rwd_p[:])
    else:
        def _fin(rwT_n=rwT_n, mem_nrm_n=mem_nrm_n, t=t):
            cnt = _readmix(t)
            V.tensor_copy(rwT_n[:], cnt[:].rearrange("p c r -> p (c r)"))
            rwd_p = psS.tile([W, R], F32, tag="s")
            for c in range(NT):
                P.matmul(rwd_p[:], mem_nrm_n[:, c, :],
                         rwT_n[:, R * c:R * (c + 1)],
                         start=(c == 0), stop=(c == NT - 1))
            S.copy(out_sb[:, t, :], rwd_p[:])
    if last:
        _fin()
    else:
        st_fin = _fin

    return dict(memT=memT_n, mem_nrm=mem_nrm_n, mnorm=mnorm_n, L=L_n,
                LT=LT_n, dL=dL_n, u_pm=u_pm_n, na_pm=na_pm_n,
                nsa=nsa_n, prec_pm=prec_pm_n, prec_fl=prec_fl_n, pbs=pbs,
                rwT=rwT_n, fin=(None if last else st_fin))


# ---------------------------------------------------------------------------
_NC_CACHE = {}


def _get_nc():
    if "nc" not in _NC_CACHE:
        _NC_CACHE["nc"] = build_nc()
    return _NC_CACHE["nc"]


def _consts():
    ident = np.eye(128, dtype=np.float32)
    return (ident,)


def make_in_maps(controller_output, W_if, b_if, memory0):
    (ident,) = _consts()
    maps = []
    for b in range(B):
        maps.append({
            "co": np.ascontiguousarray(controller_output[b]),
            "wif": np.ascontiguousarray(W_if),
            "bif": np.ascontiguousarray(b_if.reshape(1, IF)),
            "mem0": np.ascontiguousarray(memory0[b]),
            "ident": ident,
        })
    return maps


def kernel(controller_output, W_if, b_if, memory0):
    from concourse.bass_utils import run_bass_kernel_spmd
    controller_output = np.asarray(controller_output, dtype=np.float32)
    W_if = np.asarray(W_if, dtype=np.float32)
    b_if = np.asarray(b_if, dtype=np.float32)
    memory0 = np.asarray(memory0, dtype=np.float32)
    nc = _get_nc()
    maps = make_in_maps(controller_output, W_if, b_if, memory0)
    # Retry once on non-finite output: a stale device (e.g. after an
    # earlier aborted run) can poison cores on the first dispatch.
    for _ in range(2):
        res = run_bass_kernel_spmd(nc, maps, core_ids=list(range(B)))
        out = np.stack([res.results[b]["out"] for b in range(B)], axis=0)
        if np.isfinite(out).all():
            break
    return out


if __name__ == "__main__":
    mode = sys.argv[1] if len(sys.argv) > 1 else "sim"
    sys.path.insert(0, "/root/problem")
    import jax
    with jax.default_device(jax.devices("cpu")[0]):
        import reference
        inputs = {k: np.asarray(v) for k, v in reference.setup_inputs().items()}
        expected = np.asarray(reference.reference(**inputs))

    if mode == "sim":
        from concourse.bass_interp import CoreSim
        nc = build_nc()
        maps = make_in_maps(inputs["controller_output"], inputs["W_if"],
                            inputs["b_if"], inputs["memory0"])
        sim = CoreSim(nc)
        for k, v in maps[0].items():
            sim.tensor(k)[:] = v
        sim.simulate()
        got = sim.tensor("out").copy()
        exp = expected[0]
        err = np.abs(got - exp)
        rel = np.linalg.norm(got - exp) / (np.linalg.norm(exp) + 1e-12)
        print("sim modeled time (ns):", sim.time)
        print("max abs err:", err.max(), " rel err:", rel)
    else:
        got = kernel(**inputs)
        rel = np.linalg.norm(got - expected) / (np.linalg.norm(expected) + 1e-12)
        print("max abs err:", np.abs(got - expected).max(), " rel err:", rel)


# revision 16
# speedup vs baseline: 1.7916x; 1.0040x over previous
"""DNC MemoryAccess kernel for Trainium2 (Bass/Tile), data-parallel over batch.

Shapes (hardcoded): B=8, T=16, C=1024, IFACE=471, N=512, WORD=64, R=4, NW=1.
Each of the 8 cores processes one batch element; all recurrent state stays
SBUF-resident across the T=16 sequential steps.

Design (vs the fp32 predecessor, 326us -> 192us modeled):
- the temporal link matrix L and its transpose LT are held in bf16; the
  elementwise recurrence L' = (1-w_i-w_j)L + w_i p_j runs as fast-mode
  tensor_scalar ops (0.25x DVE cycles in bf16) for w1 = w_j-(1-w_i) and the
  rank-1 terms, with the tensor_tensor multiplies/combines split across
  Pool and DVE,
- the link diagonal is never zeroed in-place: the scalar recurrence
  d' = (1-2w)d + w p is tracked in [128,NT] and its contribution is
  subtracted from the fwd/bwd PE matmul results,
- broadcast matmuls (ww, prec over partitions/words) use bf16 operands
  (1 PE cycle/row vs 4 for fp32); the usage broadcast for the allocation
  sort compare stays exact fp32 so sort ties match the fp32 reference,
- ln(usage) for the allocation cumprod and the memory-norm rsqrt use the
  Activation-table Ln/Exp; get_activation_tables is patched (membership
  only, original set order preserved) so Exp and Ln resolve to the one
  act-func set that contains both, hoisting the 1.3us table load out of
  the step loop,
- emission order is tuned for the per-engine in-order queues: the read
  softmax, rwTm scaling and dL tracker are emitted so the DVE queue never
  head-blocks the ww chain of the next step; the precedence flat vector
  is produced by PE transposes + one Act copy,
- float32r matmuls are NOT used: they fail neuronxcc BIR verification in
  this toolchain (sim accepts them; hardware compile rejects).

Precision: bf16 rounds the link matrices and the write/erase broadcasts
(~1e-3 relative on the output); usage comparisons stay exact fp32 so the
allocation sort matches the reference except for genuine fp32 ties (b=7
carries one, same as the fp32 baseline).
"""
import sys

sys.path.insert(0, "/opt/trn_rl_repo")

import numpy as np

import concourse.bacc as bacc
import concourse.bass as bass
import concourse.mybir as mybir
import concourse.tile as tile

F32 = mybir.dt.float32
F32R = mybir.dt.float32r
BF16 = mybir.dt.bfloat16
I32 = mybir.dt.int32
AF = mybir.ActivationFunctionType
OP = mybir.AluOpType

B, T, C, IF = 8, 16, 1024, 471
N, W, R = 512, 64, 4
NT = N // 128

O_RK, O_RS, O_WK, O_WS = 0, 256, 260, 324
O_ER, O_WV, O_FG, O_AG, O_WG, O_MD = 325, 389, 453, 457, 458, 459


def fr(ap):
    return ap


# Prefer the activation-function set that contains Exp AND Ln (plus
# Copy/Square/Sign), so the per-step Exp/Ln mix resolves to one table and the
# compiler hoists a single LoadActFuncSet out of the step loop instead of
# thrashing 1283ns loads between exp-only and ln-only sets.
_ORIG_GET_ACT_TABLES = None


def _patch_act_tables():
    global _ORIG_GET_ACT_TABLES
    if _ORIG_GET_ACT_TABLES is not None:
        return
    import concourse.hw_specs as hw_specs
    _ORIG_GET_ACT_TABLES = hw_specs.get_activation_tables

    def pinned(arch):
        tabs = dict(_ORIG_GET_ACT_TABLES(arch))
        pref = "natural_log_exp_and_others"
        if pref not in tabs:
            return tabs
        exp_ln = {mybir.ActivationFunctionType.Exp,
                  mybir.ActivationFunctionType.Ln}
        out = {}
        for k, v in tabs.items():
            out[k] = set(v) if k == pref else set(v) - exp_ln
        return out

    bacc.get_activation_tables = pinned


def build_nc():
    _patch_act_tables()
    nc = bacc.Bacc("TRN2", target_bir_lowering=False, debug=False, num_devices=8)

    co_d = nc.declare_dram_parameter("co", [T, C], F32, isOutput=False)
    w_d = nc.declare_dram_parameter("wif", [C, IF], F32, isOutput=False)
    b_d = nc.declare_dram_parameter("bif", [1, IF], F32, isOutput=False)
    m0_d = nc.declare_dram_parameter("mem0", [N, W], F32, isOutput=False)
    ident_d = nc.declare_dram_parameter("ident", [128, 128], F32, isOutput=False)
    out_d = nc.declare_dram_parameter("out", [T, R, W], F32, isOutput=True)

    with tile.TileContext(nc) as tc:
        with (
            nc.allow_low_precision(reason="bf16 link + f32r broadcasts stay"
                                   " within the 2e-2 gate"),
            tc.tile_pool(name="const", bufs=1) as cp,
            tc.tile_pool(name="state", bufs=2) as sp,
            tc.tile_pool(name="work", bufs=2) as wp,
            tc.tile_pool(name="psBig", bufs=1, space="PSUM") as psB,
            tc.tile_pool(name="psMem", bufs=1, space="PSUM") as psM,
            tc.tile_pool(name="psS", bufs=2, space="PSUM") as psS,
        ):
            _build_body(nc, tc, cp, sp, wp, psB, psM, psS,
                        co_d, w_d, b_d, m0_d, ident_d, out_d)
    nc.compile()
    return nc


def _build_body(nc, tc, cp, sp, wp, psB, psM, psS,
                co_d, w_d, b_d, m0_d, ident_d, out_d):
    V, S, P, G_, DMA = nc.vector, nc.scalar, nc.tensor, nc.gpsimd, nc.sync

    # ---------------- constants ----------------
    ident = cp.tile([128, 128], F32)
    DMA.dma_start(ident[:], ident_d[:])
    ones = cp.tile([128, 128], F32)
    G_.memset(ones[:], 1.0)
    ones_b = cp.tile([1, 128], BF16)
    G_.memset(ones_b[:], 1.0)
    ident_b = cp.tile([128, 128], BF16)
    V.tensor_copy(ident_b[:], ident[:])

    # persistent per-t tables
    iface = cp.tile([T, IF], F32)          # raw iface rows
    wvR = cp.tile([1, T, W], BF16)         # write vectors, partition-0 rows
    keysc = cp.tile([W, 5, T], F32)        # scaled keys: r=0..3 read, 4 write
    neg_er = cp.tile([W, T], F32)
    gr = cp.tile([1, 6, T], F32)           # sigmoids: fg x4, ag, wg
    c1p = cp.tile([1, T], F32)
    cn1 = cp.tile([1, T], F32)
    c2 = cp.tile([1, T], F32)
    modes1 = cp.tile([1, R, T], F32)       # content-mode row per t
    mbs0 = cp.tile([128, R, T], F32)
    mbs2 = cp.tile([128, R, T], F32)
    nege0_pm = cp.tile([128, NT], F32)
    G_.memset(nege0_pm[:], 0.0)
    G_.memset(nege0_pm[0:1, 0:1], -1.0)
    out_sb = cp.tile([W, T, R], F32)

    # ---------------- prologue ----------------
    with tc.tile_pool(name="prolog", bufs=1) as pp:
        co_sb = pp.tile([T, C], F32)
        DMA.dma_start(co_sb[:], co_d[:])
        bif_sb = pp.tile([1, IF], F32)
        DMA.dma_start(bif_sb[:], b_d[:])
        w_sb = pp.tile([128, 8, IF], F32)
        for k in range(8):
            # split the 1.9MB load across two hwdge queues
            eng = DMA if k % 2 == 0 else nc.scalar
            eng.dma_start(w_sb[:, k, :], w_d[128 * k:128 * (k + 1), :])

        coT_p = psB.tile([128, 8, T], F32, tag="wb")
        for k in range(8):
            P.transpose(coT_p[:, k, :], co_sb[:, 128 * k:128 * (k + 1)],
                        ident[0:T, 0:T])
        coT = pp.tile([128, 8, T], F32)
        V.tensor_copy(coT[:], coT_p[:])

        if_p = psB.tile([T, IF], F32, tag="pb", bufs=2)
        for k in range(8):
            P.matmul(if_p[:], coT[:, k, :], w_sb[:, k, :],
                     start=(k == 0), stop=False)
        P.matmul(if_p[:], ones[0:1, 0:T], bif_sb[:],
                 start=False, stop=True)
        V.tensor_copy(iface[:], if_p[:])

        # keys [64, 5, T]: read r=0..3, write at 4
        keys_p = psB.tile([W, 5, T], F32, tag="pb", bufs=2)
        for r in range(R):
            P.transpose(keys_p[:, r, :], iface[:, O_RK + W * r:O_RK + W * (r + 1)],
                        ident[0:T, 0:T])
        P.transpose(keys_p[:, 4, :], iface[:, O_WK:O_WK + W], ident[0:T, 0:T])
        keys = pp.tile([W, 5, T], F32)
        V.tensor_copy(keys[:], keys_p[:])

        # write vectors as partition-0 rows via selector matmuls, two copies
        for h in range(2):
            wv_p = psB.tile([1, 8, W], F32, tag="pb", bufs=2, name=f"wvp{h}")
            for j in range(8):
                tt_ = 8 * h + j
                P.matmul(wv_p[0:1, j, :], ident[0:T, tt_:tt_ + 1],
                         iface[:, O_WV:O_WV + W])
            V.tensor_copy(wvR[0:1, 8 * h:8 * (h + 1), :].rearrange(
                "o t w -> o (t w)"),
                wv_p[:].rearrange("o t w -> o (t w)"))

        # erase sigmoid -> neg_er
        er_p = psS.tile([W, T], F32, tag="s")
        P.transpose(er_p[:], iface[:, O_ER:O_ER + W], ident[0:T, 0:T])
        ee = pp.tile([W, T], F32)
        S.activation(ee[:], er_p[:], AF.Exp, scale=-1.0)
        ew = pp.tile([W, T], F32)
        V.tensor_scalar(ew[:], ee[:], 1.0, None, op0=OP.add)
        er_r = pp.tile([W, T], F32)
        V.reciprocal(er_r[:], ew[:])
        V.tensor_scalar(neg_er[:], er_r[:], -1.0, None, op0=OP.mult)

        # strengths softplus: [1, 5, T] (rs x4, ws)
        sts_p = psS.tile([1, 5, T], F32, tag="s")
        for r in range(R):
            P.transpose(sts_p[0:1, r, :], iface[:, O_RS + r:O_RS + r + 1],
                        ident[0:T, 0:T])
        P.transpose(sts_p[0:1, 4, :], iface[:, O_WS:O_WS + 1], ident[0:T, 0:T])
        st_e = pp.tile([1, 5 * T], F32)
        S.activation(st_e[:], sts_p[:].rearrange("o f t -> o (f t)"), AF.Exp)
        st_w = pp.tile([1, 5 * T], F32)
        V.tensor_scalar(st_w[:], st_e[:], 1.0, None, op0=OP.add)
        st_sp = pp.tile([1, 5 * T], F32)
        S.activation(st_sp[:], st_w[:], AF.Ln)

        # key norms: rsqrt(sum keys^2) = exp(-0.5 ln)
        sqk = pp.tile([W, 5 * T], F32)
        S.activation(sqk[:], keys[:].rearrange("w f t -> w (f t)"), AF.Square)
        k2_p = psM.tile([1, 5 * T], F32, tag="wwb")
        P.matmul(k2_p[:], ones[0:W, 0:1], sqk[:])
        lk2 = pp.tile([1, 5 * T], F32)
        S.activation(lk2[:], k2_p[:], AF.Ln)
        kr = pp.tile([1, 5 * T], F32)
        S.activation(kr[:], lk2[:], AF.Exp, scale=-0.5)
        beta = pp.tile([1, 5 * T], F32)
        V.tensor_tensor(beta[:], st_sp[:], kr[:], op=OP.mult)
        kb_p = psM.tile([W, 5 * T], F32, tag="add")
        P.matmul(kb_p[:], ones[0:1, 0:W], beta[:])
        V.tensor_tensor(keysc[:].rearrange("w f t -> w (f t)"),
                        keys[:].rearrange("w f t -> w (f t)"), kb_p[:],
                        op=OP.mult)

        # gates: fg x4, ag, wg sigmoids
        gats_p = psS.tile([1, 6, T], F32, tag="s")
        for r in range(R):
            P.transpose(gats_p[0:1, r, :], iface[:, O_FG + r:O_FG + r + 1],
                        ident[0:T, 0:T])
        P.transpose(gats_p[0:1, 4, :], iface[:, O_AG:O_AG + 1], ident[0:T, 0:T])
        P.transpose(gats_p[0:1, 5, :], iface[:, O_WG:O_WG + 1], ident[0:T, 0:T])
        g_e = pp.tile([1, 6 * T], F32)
        S.activation(g_e[:], gats_p[:].rearrange("o g t -> o (g t)"), AF.Exp,
                     scale=-1.0)
        g_w = pp.tile([1, 6 * T], F32)
        V.tensor_scalar(g_w[:], g_e[:], 1.0, None, op0=OP.add)
        V.reciprocal(gr[:].rearrange("o g t -> o (g t)"), g_w[:])
        ag_t = gr[0:1, 4, :]
        wg_t = gr[0:1, 5, :]
        V.tensor_tensor(c1p[:], ag_t, wg_t, op=OP.mult)
        V.tensor_scalar(cn1[:], c1p[:], -1.0, None, op0=OP.mult)
        V.tensor_tensor(c2[:], wg_t, c1p[:], op=OP.subtract)

        # modes softmax -> rows per t
        me = pp.tile([T, 12], F32)
        S.activation(me[:], iface[:, O_MD:O_MD + 12], AF.Exp)
        me3 = me[:].rearrange("t (r m) -> t r m", m=3)
        msum = pp.tile([T, R], F32)
        V.tensor_tensor(msum[:], me3[:, :, 0], me3[:, :, 1], op=OP.add)
        V.tensor_tensor(msum[:], msum[:], me3[:, :, 2], op=OP.add)
        mrcp = pp.tile([T, R], F32)
        V.reciprocal(mrcp[:], msum[:])
        mn = pp.tile([T, 12], F32)
        mn3 = mn[:].rearrange("t (m r) -> t m r", r=R)
        me3b = me[:].rearrange("t (r m) -> t m r", m=3)
        for m in range(3):
            V.tensor_tensor(mn3[:, m, :], me3b[:, m, :], mrcp[:], op=OP.mult)
        # three m-blocks at base partition 0: modes0/1/2 [4, T]
        mblk_p = psS.tile([R, 3, T], F32, tag="s")
        for m in range(3):
            P.transpose(mblk_p[:, m, :], mn[:, 4 * m:4 * (m + 1)],
                        ident[0:T, 0:T])
        mblk = pp.tile([R, 3, T], F32)
        V.tensor_copy(mblk[:], mblk_p[:])
        m1sel_p = psS.tile([1, R, T], F32, tag="s")
        for r in range(R):
            P.matmul(m1sel_p[0:1, r, :], ident[0:R, r:r + 1], mblk[:, 1, :])
        V.tensor_copy(modes1[:].rearrange("o r t -> o (r t)"),
                      m1sel_p[:].rearrange("o r t -> o (r t)"))
        # flatten rows r of m-block 0/2 onto partition 0 via selector matmuls
        mrows_p = psS.tile([1, 2, R, T], F32, tag="s")
        for r in range(R):
            P.matmul(mrows_p[0:1, 0, r, :], ident[0:R, r:r + 1], mblk[:, 0, :])
            P.matmul(mrows_p[0:1, 1, r, :], ident[0:R, r:r + 1], mblk[:, 2, :])
        mrows = pp.tile([1, 2, R, T], F32)
        V.tensor_copy(mrows[:].rearrange("o a r t -> o (a r t)"),
                      mrows_p[:].rearrange("o a r t -> o (a r t)"))
        mb0_p = psB.tile([128, R * T], F32, tag="wb")
        P.matmul(mb0_p[:], ones[0:1, :], mrows[0:1, 0, :, :])
        V.tensor_copy(mbs0[:].rearrange("p r t -> p (r t)"), mb0_p[:])
        mb2_p = psB.tile([128, R * T], F32, tag="pb", bufs=2)
        P.matmul(mb2_p[:], ones[0:1, :], mrows[0:1, 1, :, :])
        V.tensor_copy(mbs2[:].rearrange("p r t -> p (r t)"), mb2_p[:])

    # ---------------- initial state ----------------
    mem_nrm = sp.tile([128, NT, W], F32, tag="mem_nrm")
    for c in range(NT):
        DMA.dma_start(mem_nrm[:, c, :],
                      m0_d[128 * c:128 * (c + 1), :])
    memT_p = psB.tile([W, N], F32, tag="wb")
    for c in range(NT):
        P.transpose(memT_p[:, 128 * c:128 * (c + 1)],
                    mem_nrm[:, c, :], ident[:])
    memT = sp.tile([W, N], F32, tag="memT")
    V.tensor_copy(memT[:], memT_p[:])

    # initial norm: PM-layout sqn -> msq -> Ln/Exp
    sqn0 = wp.tile([128, NT, W], F32, tag="sqn")
    G_.tensor_tensor(sqn0[:], mem_nrm[:], mem_nrm[:], op=OP.mult)
    msq0 = wp.tile([128, NT], F32, tag="msq")
    V.tensor_reduce(msq0[:], sqn0[:], axis=mybir.AxisListType.X, op=OP.add)
    lms0 = wp.tile([128, NT], F32, tag="lms")
    S.activation(lms0[:], msq0[:], AF.Ln)
    mnorm_i = sp.tile([128, NT], F32, tag="mnorm")
    S.activation(mnorm_i[:], lms0[:], AF.Exp, scale=-0.5)

    L = sp.tile([128, NT, N], BF16, tag="L")
    G_.memset(L[:], 0.0)
    LT0 = sp.tile([128, NT, N], BF16, tag="LT")
    G_.memset(LT0[:], 0.0)
    dL0 = sp.tile([128, NT], F32, tag="dL")
    G_.memset(dL0[:], 0.0)

    st = dict(memT=memT, mem_nrm=mem_nrm, mnorm=mnorm_i, L=L, LT=LT0,
              dL=dL0, u_pm=None, prec_pm=None, prec_fl=None,
              pbs=None, rwT=None)

    for t in range(T):
        st = _step(nc, t, st, cp, sp, wp, psB, psM, psS,
                   ident, ident_b, ones, ones_b, iface, wvR, keysc, neg_er,
                   gr, c1p, cn1, c2, modes1, mbs0, mbs2, nege0_pm, out_sb)

    DMA.dma_start(out_d[:].rearrange("t r w -> w t r"), out_sb[:])


def _step(nc, t, st, cp, sp, wp, psB, psM, psS,
          ident, ident_b, ones, ones_b, iface, wvR, keysc, neg_er, gr, c1p,
          cn1, c2, modes1, mbs0, mbs2, nege0_pm, out_sb):
    V, S, P, G_, DMA = nc.vector, nc.scalar, nc.tensor, nc.gpsimd, nc.sync
    memT, mem_nrm, mnorm = st["memT"], st["mem_nrm"], st["mnorm"]
    L, LT, dL, u_pm = st["L"], st["LT"], st["dL"], st["u_pm"]
    prec_pm, prec_fl, pbs, rwT = (st["prec_pm"], st["prec_fl"], st["pbs"],
                                  st["rwT"])
    last = (t == T - 1)

    if t == 0:
        na_pm, nsa = nege0_pm, None
    else:
        na_pm, nsa = st["na_pm"], st["nsa"]

    # ---- pbs broadcast (prev-step prec; runs at step start) ----
    if t > 0:
        pb_p = psB.tile([128, N], F32, tag="pb", bufs=2)
        P.matmul(pb_p[:], ones_b[0:1, :], prec_fl[:])
        pbs = wp.tile([128, N], BF16, tag="pbs")
        S.activation(pbs[:], pb_p[:], AF.Copy)

    # ---- write content softmax (PM) ----
    wdots_p = psS.tile([128, NT], F32, tag="s")
    for b in range(NT):
        P.matmul(wdots_p[:, b:b + 1], memT[:, 128 * b:128 * (b + 1)],
                 keysc[:, 4, t:t + 1])
    wlog = wp.tile([128, NT], F32, tag="wlog")
    V.tensor_tensor(wlog[:], wdots_p[:], mnorm[:], op=OP.mult)
    wexp_pm = wp.tile([128, NT], F32, tag="wexp")
    S.activation(wexp_pm[:], wlog[:], AF.Exp)
    wps_p = psS.tile([1, NT], F32, tag="s")
    P.matmul(wps_p[:], ones[:, 0:1], wexp_pm[:])
    wsum = wp.tile([1, 1], F32, tag="wsum")
    V.tensor_reduce(wsum[:], wps_p[:], axis=mybir.AxisListType.X, op=OP.add)
    wrs = wp.tile([1, 1], F32, tag="wrs")
    V.reciprocal(wrs[:], wsum[:])
    cw = wp.tile([1, 1], F32, tag="cw")
    V.tensor_tensor(cw[:], wrs[:], c2[0:1, t:t + 1], op=OP.mult)

    # ---- ww assembly (PM) ----
    cn1b_p = psS.tile([128, 1], F32, tag="s")
    P.matmul(cn1b_p[:], ones[0:1, :], cn1[0:1, t:t + 1])
    cwb_p = psS.tile([128, 1], F32, tag="s")
    P.matmul(cwb_p[:], ones[0:1, :], cw[:])
    wwx = wp.tile([128, NT], F32, tag="wwx")
    V.tensor_scalar(wwx[:], na_pm[:], cn1b_p[:, 0:1], None, op0=OP.mult)
    ww_pm = wp.tile([128, NT], F32, tag="wwpm")
    V.scalar_tensor_tensor(ww_pm[:], wexp_pm[:], cwb_p[:, 0:1], wwx[:],
                           op0=OP.mult, op1=OP.add)
    if t > 0:
        omw_pm = wp.tile([128, NT], F32, tag="omw")
        V.tensor_scalar(omw_pm[:], ww_pm[:], -1.0, 1.0, op0=OP.mult,
                        op1=OP.add)
    if st.get("fin") is not None:
        st["fin"]()
        st["fin"] = None

    wwpm_b = wp.tile([128, NT], BF16, tag="wwpmb")
    V.tensor_copy(wwpm_b[:], ww_pm[:])
    ww_tp = psS.tile([1, N], BF16, tag="s")
    for c in range(NT):
        P.transpose(ww_tp[0:1, 128 * c:128 * (c + 1)], wwpm_b[:, c:c + 1],
                    ident_b[:])
    ww_fl = wp.tile([1, N], BF16, tag="wwfl")
    S.copy(ww_fl[:], ww_tp[:])

    # ---- wbs broadcast (bf16, for the link ts ops) ----
    if t > 0:
        wb_p = psB.tile([128, N], F32, tag="wb")
        P.matmul(wb_p[:], ones_b[0:1, :], ww_fl[:])
        wbs = wp.tile([128, N], BF16, tag="wbs")
        S.activation(wbs[:], wb_p[:], AF.Copy)

    # ---- memory head ----
    wwb_p = psM.tile([W, N], F32, tag="wwb")
    P.matmul(wwb_p[:], ones_b[0:1, 0:W], ww_fl[:])
    add_p = psM.tile([W, N], F32, tag="add")
    P.matmul(add_p[:], wvR[0:1, t, :], ww_fl[:])
    keep = wp.tile([W, N], F32, tag="keep")
    V.tensor_scalar(keep[:], wwb_p[:], neg_er[:, t:t + 1], 1.0,
                    op0=OP.mult, op1=OP.add)
    m1 = wp.tile([W, N], F32, tag="m1")
    G_.tensor_tensor(m1[:], memT[:], keep[:], op=OP.mult)

    # ---- usage update ----
    if last:
        u_pm_n = u_pm
    else:
        u_pm_n = sp.tile([128, NT], F32, tag="u_pm")
        if t == 0:
            V.tensor_copy(u_pm_n[:], ww_pm[:])
        else:
            fgb_p = psS.tile([128, R], F32, tag="s")
            P.matmul(fgb_p[:], ones[0:1, :], gr[0:1, 0:R, t])
            yyT = wp.tile([128, NT, R], F32, tag="yyT")
            V.scalar_tensor_tensor(
                yyT[:], fgb_p[:, None, :].broadcast_to([128, NT, R]), -1.0,
                rwT[:].rearrange("p (c r) -> p c r", r=R),
                op0=OP.mult, op1=OP.mult)
            om = wp.tile([128, NT, R], F32, tag="om")
            V.tensor_scalar(om[:], yyT[:], 1.0, None, op0=OP.add)
            p1u = wp.tile([128, NT], F32, tag="p1u")
            G_.tensor_tensor(p1u[:], om[:, :, 0], om[:, :, 1], op=OP.mult)
            p2u = wp.tile([128, NT], F32, tag="p2u")
            G_.tensor_tensor(p2u[:], om[:, :, 2], om[:, :, 3], op=OP.mult)
            psi = wp.tile([128, NT], F32, tag="psi")
            G_.tensor_tensor(psi[:], p1u[:], p2u[:], op=OP.mult)
            omu = wp.tile([128, NT], F32, tag="omu")
            V.tensor_scalar(omu[:], u_pm[:], -1.0, 1.0, op0=OP.mult,
                            op1=OP.add)
            tn = wp.tile([128, NT], F32, tag="tn")
            V.scalar_tensor_tensor(tn[:], ww_pm[:], 1.0, omu[:],
                                   op0=OP.subtract, op1=OP.mult)
            V.scalar_tensor_tensor(u_pm_n[:], tn[:], 1.0, psi[:],
                                   op0=OP.add, op1=OP.mult)

    # ---- allocation compare inputs (flat u + broadcast; exact fp32) ----
    if not last:
        u_tp = psS.tile([1, N], F32, tag="s")
        for c in range(NT):
            P.transpose(u_tp[0:1, 128 * c:128 * (c + 1)], u_pm_n[:, c:c + 1],
                        ident[:])
        u_fl_n = wp.tile([1, N], F32, tag="ufl")
        S.copy(u_fl_n[:], u_tp[:])
        ub_p = psM.tile([128, N], F32, tag="wwb")
        P.matmul(ub_p[:], ones[0:1, :], u_fl_n[:])
        ubs = wp.tile([128, N], F32, tag="ubs")
        S.copy(ubs[:], ub_p[:])
        ucl = wp.tile([128, NT], F32, tag="ucl")
        V.tensor_scalar(ucl[:], u_pm_n[:], 1e-38, None, op0=OP.max)
        lnu = wp.tile([128, NT], F32, tag="lnu")
        S.activation(lnu[:], ucl[:], AF.Ln)

    # ---- prec update ----
    if not last:
        prec_pm_n = sp.tile([128, NT], F32, tag="prec_pm")
        if t == 0:
            V.tensor_copy(prec_pm_n[:], ww_pm[:])
        else:
            swa = wp.tile([1, 1], F32, tag="swa")
            G_.tensor_tensor(swa[:], nsa[:], cn1[0:1, t:t + 1], op=OP.mult)
            sw = wp.tile([1, 1], F32, tag="sw")
            G_.tensor_tensor(sw[:], swa[:], c2[0:1, t:t + 1], op=OP.add)
            omsw = wp.tile([1, 1], F32, tag="omsw")
            V.tensor_scalar(omsw[:], sw[:], -1.0, 1.0, op0=OP.mult,
                            op1=OP.add)
            omsw_p = psS.tile([128, 1], F32, tag="s")
            P.matmul(omsw_p[:], ones[0:1, :], omsw[:])
            V.scalar_tensor_tensor(prec_pm_n[:], prec_pm[:], omsw_p[:, 0:1],
                                   ww_pm[:], op0=OP.mult, op1=OP.add)
        p_tp = psS.tile([1, N], F32, tag="s")
        for c in range(NT):
            P.transpose(p_tp[0:1, 128 * c:128 * (c + 1)], prec_pm_n[:, c:c + 1],
                        ident[:])
        prec_fl_n = sp.tile([1, N], BF16, tag="prec_fl")
        S.copy(prec_fl_n[:], p_tp[:])
    else:
        prec_pm_n, prec_fl_n = prec_pm, prec_fl

    # ---- mode-scaled read weights + link diagonal tracker ----

    # ---- link loop with interleaved memT_n / Gt compares ----
    comb_eng = [(G_, G_), (G_, V), (G_, V), (G_, V)]
    if t == 0:
        L_n, LT_n = L, LT
        memT_n = sp.tile([W, N], F32, tag="memT")
        V.tensor_tensor(memT_n[:], m1[:], add_p[:], op=OP.add)
        if not last:
            Gt_n = wp.tile([128, NT, N], F32, tag="G", bufs=1)
            for c in range(NT):
                V.tensor_scalar(Gt_n[:, c, :], ubs[:], u_pm_n[:, c:c + 1],
                                None, op0=OP.is_gt)
    else:
        L_n = sp.tile([128, NT, N], BF16, tag="L")
        LT_n = sp.tile([128, NT, N], BF16, tag="LT")
        memT_n = sp.tile([W, N], F32, tag="memT")
        if not last:
            Gt_n = wp.tile([128, NT, N], F32, tag="G", bufs=1)
        for c in range(NT):
            w1 = wp.tile([128, N], BF16, tag=f"w1_{c % 2}")
            V.tensor_scalar(w1[:], wbs[:], omw_pm[:, c:c + 1], None,
                            op0=OP.subtract)
            p1 = wp.tile([128, N], BF16, tag=f"p1_{c % 2}")
            V.tensor_scalar(p1[:], pbs[:], ww_pm[:, c:c + 1], None,
                            op0=OP.mult)
            p1T = wp.tile([128, N], BF16, tag=f"p1T_{c % 2}")
            V.tensor_scalar(p1T[:], wbs[:], prec_pm[:, c:c + 1], None,
                            op0=OP.mult)
            t1 = wp.tile([128, N], BF16, tag=f"t1_{c % 2}")
            G_.tensor_tensor(t1[:], w1[:], L[:, c, :], op=OP.mult)
            t1T = wp.tile([128, N], BF16, tag=f"t1T_{c % 2}")
            G_.tensor_tensor(t1T[:], w1[:], LT[:, c, :], op=OP.mult)
            eL, eLT = comb_eng[c]
            eL.tensor_tensor(L_n[:, c, :], p1[:], t1[:], op=OP.subtract)
            eLT.tensor_tensor(LT_n[:, c, :], p1T[:], t1T[:], op=OP.subtract)
            if c > 0 and not last:
                cc = c - 1
                V.tensor_scalar(Gt_n[:, cc, :], ubs[:],
                                u_pm_n[:, cc:cc + 1], None, op0=OP.is_gt)
        V.tensor_tensor(memT_n[:], m1[:], add_p[:], op=OP.add)
        if not last:
            V.tensor_scalar(Gt_n[:, 3, :], ubs[:], u_pm_n[:, 3:4],
                            None, op0=OP.is_gt)

    if t > 0:
        wp_pm = wp.tile([128, NT], F32, tag="wppm")
        G_.tensor_tensor(wp_pm[:], ww_pm[:], prec_pm[:], op=OP.mult)
        dmul = wp.tile([128, NT], F32, tag="dmul")
        V.tensor_scalar(dmul[:], ww_pm[:], -2.0, 1.0, op0=OP.mult, op1=OP.add)
        dL_n = sp.tile([128, NT], F32, tag="dL")
        V.scalar_tensor_tensor(dL_n[:], dL[:], 1.0, dmul[:],
                               op0=OP.mult, op1=OP.mult)
        G_.tensor_tensor(dL_n[:], dL_n[:], wp_pm[:], op=OP.add)
    else:
        dL_n = dL
    if t > 0:
        rwTm0 = wp.tile([128, NT * R], BF16, tag="rwTm0")
        V.tensor_tensor(rwTm0[:].rearrange("p (c r) -> p c r", r=R),
                        rwT[:].rearrange("p (c r) -> p c r", r=R),
                        mbs0[:, None, :, t].broadcast_to([128, NT, R]),
                        op=OP.mult)
        rwTm2 = wp.tile([128, NT * R], BF16, tag="rwTm2")
        V.tensor_tensor(rwTm2[:].rearrange("p (c r) -> p c r", r=R),
                        rwT[:].rearrange("p (c r) -> p c r", r=R),
                        mbs2[:, None, :, t].broadcast_to([128, NT, R]),
                        op=OP.mult)

    # ---- memory norm chain ----
    mem_nrm_p = psS.tile([128, NT, W], F32, tag="mn", bufs=1)
    for c in range(NT):
        P.transpose(mem_nrm_p[:, c, :], memT_n[:, 128 * c:128 * (c + 1)],
                    ident[0:W, 0:W])
    sqn = wp.tile([128, NT, W], F32, tag="sqn")
    S.activation(sqn[:], mem_nrm_p[:], AF.Square)
    msq = wp.tile([128, NT], F32, tag="msq")
    V.tensor_reduce(msq[:], sqn[:], axis=mybir.AxisListType.X, op=OP.add)
    mem_nrm_n = sp.tile([128, NT, W], F32, tag="mem_nrm")
    S.copy(mem_nrm_n[:], mem_nrm_p[:])
    lms = wp.tile([128, NT], F32, tag="lms")
    S.activation(lms[:], msq[:], AF.Ln)
    mnorm_n = sp.tile([128, NT], F32, tag="mnorm")
    S.activation(mnorm_n[:], lms[:], AF.Exp, scale=-0.5)

    # ---- allocation log-sum ----
    if last:
        na_pm_n, nsa_n = None, None
    else:
        sT_p = psS.tile([128, NT], F32, tag="s")
        for b in range(NT):
            for c in range(NT):
                P.matmul(sT_p[:, b:b + 1], Gt_n[:, c, 128 * b:128 * (b + 1)],
                         lnu[:, c:c + 1], start=(c == 0),
                         stop=(c == NT - 1))
        es_pm = wp.tile([128, NT], F32, tag="espm")
        S.activation(es_pm[:], sT_p[:], AF.Exp)
        na_pm_n = wp.tile([128, NT], F32, tag="napm")
        if t < T - 2:
            nap = wp.tile([128, 1], F32, tag="nap")
            V.scalar_tensor_tensor(na_pm_n[:], u_pm_n[:], 1.0, es_pm[:],
                                   op0=OP.subtract, op1=OP.mult,
                                   accum_out=nap[:])
        else:
            V.scalar_tensor_tensor(na_pm_n[:], u_pm_n[:], 1.0, es_pm[:],
                                   op0=OP.subtract, op1=OP.mult)
        if t < T - 2:
            nsa_p = psS.tile([1, 1], F32, tag="s")
            P.matmul(nsa_p[:], nap[:], ones[:, 0:1])
            nsa_n = wp.tile([1, 1], F32, tag="nsa")
            V.tensor_copy(nsa_n[:], nsa_p[:])
        else:
            nsa_n = None

    # ---- read content (PM) ----
    rdots_p = psS.tile([128, NT * R], F32, tag="s")
    for b in range(NT):
        P.matmul(rdots_p[:, R * b:R * (b + 1)],
                 memT_n[:, 128 * b:128 * (b + 1)], keysc[:, 0:4, t])
    rlog = wp.tile([128, NT, R], F32, tag="rlog")
    V.tensor_tensor(rlog[:],
                    rdots_p[:].rearrange("p (c r) -> p c r", r=R),
                    mnorm_n[:, :, None].broadcast_to([128, NT, R]),
                    op=OP.mult)
    rexp_pm = wp.tile([128, NT * R], F32, tag="rexp")
    S.activation(rexp_pm[:], rlog[:].rearrange("p c r -> p (c r)"), AF.Exp)
    rps_p = psS.tile([1, NT * R], F32, tag="s")
    P.matmul(rps_p[:], ones[:, 0:1], rexp_pm[:])

    def _readmix(t, rps_p=rps_p, rexp_pm=rexp_pm):
        rsum = wp.tile([1, R], F32, tag="rsum")
        V.tensor_reduce(rsum[:], rps_p[:].rearrange("o (c r) -> o r c", r=R),
                        axis=mybir.AxisListType.X, op=OP.add)
        rsr = wp.tile([1, R], F32, tag="rsr")
        V.reciprocal(rsr[:], rsum[:])
        s1c = wp.tile([1, R], F32, tag="s1c")
        V.tensor_tensor(s1c[:], rsr[:], modes1[0:1, :, t], op=OP.mult)
        s1cb_p = psS.tile([128, R], F32, tag="s")
        P.matmul(s1cb_p[:], ones[0:1, :], s1c[:])
        cnt = wp.tile([128, NT, R], F32, tag="cnt")
        V.tensor_tensor(cnt[:], rexp_pm[:].rearrange("p (c r) -> p c r", r=R),
                        s1cb_p[:, None, :].broadcast_to([128, NT, R]),
                        op=OP.mult)
        return cnt

    rwT_n = sp.tile([128, NT * R], F32, tag="rwT")
    if t > 0:
        def _fin(rwT_n=rwT_n, mem_nrm_n=mem_nrm_n, dL_n=dL_n,
                 L_n=L_n, LT_n=LT_n, rwTm0=rwTm0, rwTm2=rwTm2, t=t):
            cnt = _readmix(t)
            s02 = wp.tile([128, NT, R], F32, tag="s02")
            G_.tensor_tensor(s02[:],
                             rwTm0[:].rearrange("p (c r) -> p c r", r=R),
                             rwTm2[:].rearrange("p (c r) -> p c r", r=R),
                             op=OP.add)
            corr = wp.tile([128, NT, R], F32, tag="corr")
            V.tensor_tensor(corr[:], s02[:],
                            dL_n[:, :, None].broadcast_to([128, NT, R]),
                            op=OP.mult)
            cnt2 = wp.tile([128, NT, R], F32, tag="cnt2")
            G_.tensor_tensor(cnt2[:], cnt[:], corr[:], op=OP.subtract)
            rwT_p = psS.tile([128, NT * R], F32, tag="s")
            for b in range(NT):
                blk = slice(128 * b, 128 * (b + 1))
                for c in range(NT):
                    P.matmul(rwT_p[:, R * b:R * (b + 1)], L_n[:, c, blk],
                             rwTm0[:, R * c:R * (c + 1)],
                             start=(c == 0), stop=False)
                for c in range(NT):
                    P.matmul(rwT_p[:, R * b:R * (b + 1)], LT_n[:, c, blk],
                             rwTm2[:, R * c:R * (c + 1)],
                             start=False, stop=(c == NT - 1))
            V.tensor_tensor(rwT_n[:], cnt2[:].rearrange("p c r -> p (c r)"),
                            rwT_p[:], op=OP.add)
            rwd_p = psS.tile([W, R], F32, tag="s")
            for c in range(NT):
                P.matmul(rwd_p[:], mem_nrm_n[:, c, :],
                         rwT_n[:, R * c:R * (c + 1)],
                         start=(c == 0), stop=(c == NT - 1))
            S.copy(out_sb[:, t, :], rwd_p[:])
    else:
        def _fin(rwT_n=rwT_n, mem_nrm_n=mem_nrm_n, t=t):
            cnt = _readmix(t)
            V.tensor_copy(rwT_n[:], cnt[:].rearrange("p c r -> p (c r)"))
            rwd_p = psS.tile([W, R], F32, tag="s")
            for c in range(NT):
                P.matmul(rwd_p[:], mem_nrm_n[:, c, :],
                         rwT_n[:, R * c:R * (c + 1)],
                         start=(c == 0), stop=(c == NT - 1))
            S.copy(out_sb[:, t, :], rwd_p[:])
    if last:
        _fin()
    else:
        st_fin = _fin

    return dict(memT=memT_n, mem_nrm=mem_nrm_n, mnorm=mnorm_n, L=L_n,
                LT=LT_n, dL=dL_n, u_pm=u_pm_n, na_pm=na_pm_n,
                nsa=nsa_n, prec_pm=prec_pm_n, prec_fl=prec_fl_n, pbs=pbs,
                rwT=rwT_n, fin=(None if last else st_fin))


# ---------------------------------------------------------------------------
_NC_CACHE = {}


def _get_nc():
    if "nc" not in _NC_CACHE:
        _NC_CACHE["nc"] = build_nc()
    return _NC_CACHE["nc"]


def _consts():
    ident = np.eye(128, dtype=np.float32)
    return (ident,)


def make_in_maps(controller_output, W_if, b_if, memory0):
    (ident,) = _consts()
    maps = []
    for b in range(B):
        maps.append({
            "co": np.ascontiguousarray(controller_output[b]),
            "wif": np.ascontiguousarray(W_if),
            "bif": np.ascontiguousarray(b_if.reshape(1, IF)),
            "mem0": np.ascontiguousarray(memory0[b]),
            "ident": ident,
        })
    return maps


def kernel(controller_output, W_if, b_if, memory0):
    from concourse.bass_utils import run_bass_kernel_spmd
    controller_output = np.asarray(controller_output, dtype=np.float32)
    W_if = np.asarray(W_if, dtype=np.float32)
    b_if = np.asarray(b_if, dtype=np.float32)
    memory0 = np.asarray(memory0, dtype=np.float32)
    nc = _get_nc()
    maps = make_in_maps(controller_output, W_if, b_if, memory0)
    # Retry once on non-finite output: a stale device (e.g. after an
    # earlier aborted run) can poison cores on the first dispatch.
    for _ in range(2):
        res = run_bass_kernel_spmd(nc, maps, core_ids=list(range(B)))
        out = np.stack([res.results[b]["out"] for b in range(B)], axis=0)
        if np.isfinite(out).all():
            break
    return out


if __name__ == "__main__":
    mode = sys.argv[1] if len(sys.argv) > 1 else "sim"
    sys.path.insert(0, "/root/problem")
    import jax
    with jax.default_device(jax.devices("cpu")[0]):
        import reference
        inputs = {k: np.asarray(v) for k, v in reference.setup_inputs().items()}
        expected = np.asarray(reference.reference(**inputs))

    if mode == "sim":
        from concourse.bass_interp import CoreSim
        nc = build_nc()
        maps = make_in_maps(inputs["controller_output"], inputs["W_if"],
                            inputs["b_if"], inputs["memory0"])
        sim = CoreSim(nc)
        for k, v in maps[0].items():
            sim.tensor(k)[:] = v
        sim.simulate()
        got = sim.tensor("out").copy()
        exp = expected[0]
        err = np.abs(got - exp)
        rel = np.linalg.norm(got - exp) / (np.linalg.norm(exp) + 1e-12)
        print("sim modeled time (ns):", sim.time)
        print("max abs err:", err.max(), " rel err:", rel)
    else:
        got = kernel(**inputs)
        rel = np.linalg.norm(got - expected) / (np.linalg.norm(expected) + 1e-12)
        print("max abs err:", np.abs(got - expected).max(), " rel err:", rel)


# revision 17
# speedup vs baseline: 1.8410x; 1.0276x over previous
"""DNC MemoryAccess kernel for Trainium2 (Bass/Tile), data-parallel over batch.

Shapes (hardcoded): B=8, T=16, C=1024, IFACE=471, N=512, WORD=64, R=4, NW=1.
Each of the 8 cores processes one batch element; all recurrent state stays
SBUF-resident across the T=16 sequential steps.

Design (vs the fp32 predecessor, 326us -> 192us modeled):
- the temporal link matrix L and its transpose LT are held in bf16; the
  elementwise recurrence L' = (1-w_i-w_j)L + w_i p_j runs as fast-mode
  tensor_scalar ops (0.25x DVE cycles in bf16) for w1 = w_j-(1-w_i) and the
  rank-1 terms, with the tensor_tensor multiplies/combines split across
  Pool and DVE,
- the link diagonal is never zeroed in-place: the scalar recurrence
  d' = (1-2w)d + w p is tracked in [128,NT] and its contribution is
  subtracted from the fwd/bwd PE matmul results,
- broadcast matmuls (ww, prec over partitions/words) use bf16 operands
  (1 PE cycle/row vs 4 for fp32); the usage broadcast for the allocation
  sort compare stays exact fp32 so sort ties match the fp32 reference,
- ln(usage) for the allocation cumprod and the memory-norm rsqrt use the
  Activation-table Ln/Exp; get_activation_tables is patched (membership
  only, original set order preserved) so Exp and Ln resolve to the one
  act-func set that contains both, hoisting the 1.3us table load out of
  the step loop,
- emission order is tuned for the per-engine in-order queues: the read
  softmax, rwTm scaling and dL tracker are emitted so the DVE queue never
  head-blocks the ww chain of the next step; the precedence flat vector
  is produced by PE transposes + one Act copy,
- float32r matmuls are NOT used: they fail neuronxcc BIR verification in
  this toolchain (sim accepts them; hardware compile rejects).

Precision: bf16 rounds the link matrices and the write/erase broadcasts
(~1e-3 relative on the output); usage comparisons stay exact fp32 so the
allocation sort matches the reference except for genuine fp32 ties (b=7
carries one, same as the fp32 baseline).
"""
import sys

sys.path.insert(0, "/opt/trn_rl_repo")

import numpy as np

import concourse.bacc as bacc
import concourse.bass as bass
import concourse.mybir as mybir
import concourse.tile as tile

F32 = mybir.dt.float32
F32R = mybir.dt.float32r
BF16 = mybir.dt.bfloat16
I32 = mybir.dt.int32
AF = mybir.ActivationFunctionType
OP = mybir.AluOpType

B, T, C, IF = 8, 16, 1024, 471
N, W, R = 512, 64, 4
NT = N // 128

O_RK, O_RS, O_WK, O_WS = 0, 256, 260, 324
O_ER, O_WV, O_FG, O_AG, O_WG, O_MD = 325, 389, 453, 457, 458, 459


def fr(ap):
    return ap


# Prefer the activation-function set that contains Exp AND Ln (plus
# Copy/Square/Sign), so the per-step Exp/Ln mix resolves to one table and the
# compiler hoists a single LoadActFuncSet out of the step loop instead of
# thrashing 1283ns loads between exp-only and ln-only sets.
_ORIG_GET_ACT_TABLES = None


def _patch_act_tables():
    global _ORIG_GET_ACT_TABLES
    if _ORIG_GET_ACT_TABLES is not None:
        return
    import concourse.hw_specs as hw_specs
    _ORIG_GET_ACT_TABLES = hw_specs.get_activation_tables

    def pinned(arch):
        tabs = dict(_ORIG_GET_ACT_TABLES(arch))
        pref = "natural_log_exp_and_others"
        if pref not in tabs:
            return tabs
        exp_ln = {mybir.ActivationFunctionType.Exp,
                  mybir.ActivationFunctionType.Ln}
        out = {}
        for k, v in tabs.items():
            out[k] = set(v) if k == pref else set(v) - exp_ln
        return out

    bacc.get_activation_tables = pinned


def build_nc():
    _patch_act_tables()
    nc = bacc.Bacc("TRN2", target_bir_lowering=False, debug=False, num_devices=8)

    co_d = nc.declare_dram_parameter("co", [T, C], F32, isOutput=False)
    w_d = nc.declare_dram_parameter("wif", [C, IF], F32, isOutput=False)
    b_d = nc.declare_dram_parameter("bif", [1, IF], F32, isOutput=False)
    m0_d = nc.declare_dram_parameter("mem0", [N, W], F32, isOutput=False)
    ident_d = nc.declare_dram_parameter("ident", [128, 128], F32, isOutput=False)
    out_d = nc.declare_dram_parameter("out", [T, R, W], F32, isOutput=True)

    with tile.TileContext(nc) as tc:
        with (
            nc.allow_low_precision(reason="bf16 link + f32r broadcasts stay"
                                   " within the 2e-2 gate"),
            tc.tile_pool(name="const", bufs=1) as cp,
            tc.tile_pool(name="state", bufs=2) as sp,
            tc.tile_pool(name="work", bufs=2) as wp,
            tc.tile_pool(name="psBig", bufs=1, space="PSUM") as psB,
            tc.tile_pool(name="psMem", bufs=1, space="PSUM") as psM,
            tc.tile_pool(name="psS", bufs=2, space="PSUM") as psS,
        ):
            _build_body(nc, tc, cp, sp, wp, psB, psM, psS,
                        co_d, w_d, b_d, m0_d, ident_d, out_d)
    nc.compile()
    return nc


def _build_body(nc, tc, cp, sp, wp, psB, psM, psS,
                co_d, w_d, b_d, m0_d, ident_d, out_d):
    V, S, P, G_, DMA = nc.vector, nc.scalar, nc.tensor, nc.gpsimd, nc.sync

    # ---------------- constants ----------------
    ident = cp.tile([128, 128], F32)
    DMA.dma_start(ident[:], ident_d[:])
    ones = cp.tile([128, 128], F32)
    G_.memset(ones[:], 1.0)
    ones_b = cp.tile([1, 128], BF16)
    G_.memset(ones_b[:], 1.0)
    ident_b = cp.tile([128, 128], BF16)
    V.tensor_copy(ident_b[:], ident[:])

    # persistent per-t tables
    iface = cp.tile([T, IF], F32)          # raw iface rows
    wvR = cp.tile([1, T, W], BF16)         # write vectors, partition-0 rows
    keysc = cp.tile([W, 5, T], F32)        # scaled keys: r=0..3 read, 4 write
    neg_er = cp.tile([W, T], F32)
    gr = cp.tile([1, 6, T], F32)           # sigmoids: fg x4, ag, wg
    c1p = cp.tile([1, T], F32)
    cn1 = cp.tile([1, T], F32)
    c2 = cp.tile([1, T], F32)
    modes1 = cp.tile([1, R, T], F32)       # content-mode row per t
    mbs0 = cp.tile([128, R, T], F32)
    mbs2 = cp.tile([128, R, T], F32)
    nege0_pm = cp.tile([128, NT], F32)
    G_.memset(nege0_pm[:], 0.0)
    G_.memset(nege0_pm[0:1, 0:1], -1.0)
    out_sb = cp.tile([W, T, R], F32)

    # ---------------- prologue ----------------
    with tc.tile_pool(name="prolog", bufs=1) as pp:
        co_sb = pp.tile([T, C], F32)
        DMA.dma_start(co_sb[:], co_d[:])
        bif_sb = pp.tile([1, IF], F32)
        DMA.dma_start(bif_sb[:], b_d[:])
        w_sb = pp.tile([128, 8, IF], F32)
        for k in range(8):
            # split the 1.9MB load across two hwdge queues
            eng = DMA if k % 2 == 0 else nc.scalar
            eng.dma_start(w_sb[:, k, :], w_d[128 * k:128 * (k + 1), :])

        coT_p = psB.tile([128, 8, T], F32, tag="wb")
        for k in range(8):
            P.transpose(coT_p[:, k, :], co_sb[:, 128 * k:128 * (k + 1)],
                        ident[0:T, 0:T])
        coT = pp.tile([128, 8, T], F32)
        V.tensor_copy(coT[:], coT_p[:])

        if_p = psB.tile([T, IF], F32, tag="pb", bufs=2)
        for k in range(8):
            P.matmul(if_p[:], coT[:, k, :], w_sb[:, k, :],
                     start=(k == 0), stop=False)
        P.matmul(if_p[:], ones[0:1, 0:T], bif_sb[:],
                 start=False, stop=True)
        V.tensor_copy(iface[:], if_p[:])

        # keys [64, 5, T]: read r=0..3, write at 4
        keys_p = psB.tile([W, 5, T], F32, tag="pb", bufs=2)
        for r in range(R):
            P.transpose(keys_p[:, r, :], iface[:, O_RK + W * r:O_RK + W * (r + 1)],
                        ident[0:T, 0:T])
        P.transpose(keys_p[:, 4, :], iface[:, O_WK:O_WK + W], ident[0:T, 0:T])
        keys = pp.tile([W, 5, T], F32)
        V.tensor_copy(keys[:], keys_p[:])

        # write vectors as partition-0 rows via selector matmuls, two copies
        for h in range(2):
            wv_p = psB.tile([1, 8, W], F32, tag="pb", bufs=2, name=f"wvp{h}")
            for j in range(8):
                tt_ = 8 * h + j
                P.matmul(wv_p[0:1, j, :], ident[0:T, tt_:tt_ + 1],
                         iface[:, O_WV:O_WV + W])
            V.tensor_copy(wvR[0:1, 8 * h:8 * (h + 1), :].rearrange(
                "o t w -> o (t w)"),
                wv_p[:].rearrange("o t w -> o (t w)"))

        # erase sigmoid -> neg_er
        er_p = psS.tile([W, T], F32, tag="s")
        P.transpose(er_p[:], iface[:, O_ER:O_ER + W], ident[0:T, 0:T])
        ee = pp.tile([W, T], F32)
        S.activation(ee[:], er_p[:], AF.Exp, scale=-1.0)
        ew = pp.tile([W, T], F32)
        V.tensor_scalar(ew[:], ee[:], 1.0, None, op0=OP.add)
        er_r = pp.tile([W, T], F32)
        V.reciprocal(er_r[:], ew[:])
        V.tensor_scalar(neg_er[:], er_r[:], -1.0, None, op0=OP.mult)

        # strengths softplus: [1, 5, T] (rs x4, ws)
        sts_p = psS.tile([1, 5, T], F32, tag="s")
        for r in range(R):
            P.transpose(sts_p[0:1, r, :], iface[:, O_RS + r:O_RS + r + 1],
                        ident[0:T, 0:T])
        P.transpose(sts_p[0:1, 4, :], iface[:, O_WS:O_WS + 1], ident[0:T, 0:T])
        st_e = pp.tile([1, 5 * T], F32)
        S.activation(st_e[:], sts_p[:].rearrange("o f t -> o (f t)"), AF.Exp)
        st_w = pp.tile([1, 5 * T], F32)
        V.tensor_scalar(st_w[:], st_e[:], 1.0, None, op0=OP.add)
        st_sp = pp.tile([1, 5 * T], F32)
        S.activation(st_sp[:], st_w[:], AF.Ln)

        # key norms: rsqrt(sum keys^2) = exp(-0.5 ln)
        sqk = pp.tile([W, 5 * T], F32)
        S.activation(sqk[:], keys[:].rearrange("w f t -> w (f t)"), AF.Square)
        k2_p = psM.tile([1, 5 * T], F32, tag="wwb")
        P.matmul(k2_p[:], ones[0:W, 0:1], sqk[:])
        lk2 = pp.tile([1, 5 * T], F32)
        S.activation(lk2[:], k2_p[:], AF.Ln)
        kr = pp.tile([1, 5 * T], F32)
        S.activation(kr[:], lk2[:], AF.Exp, scale=-0.5)
        beta = pp.tile([1, 5 * T], F32)
        V.tensor_tensor(beta[:], st_sp[:], kr[:], op=OP.mult)
        kb_p = psM.tile([W, 5 * T], F32, tag="add")
        P.matmul(kb_p[:], ones[0:1, 0:W], beta[:])
        V.tensor_tensor(keysc[:].rearrange("w f t -> w (f t)"),
                        keys[:].rearrange("w f t -> w (f t)"), kb_p[:],
                        op=OP.mult)

        # gates: fg x4, ag, wg sigmoids
        gats_p = psS.tile([1, 6, T], F32, tag="s")
        for r in range(R):
            P.transpose(gats_p[0:1, r, :], iface[:, O_FG + r:O_FG + r + 1],
                        ident[0:T, 0:T])
        P.transpose(gats_p[0:1, 4, :], iface[:, O_AG:O_AG + 1], ident[0:T, 0:T])
        P.transpose(gats_p[0:1, 5, :], iface[:, O_WG:O_WG + 1], ident[0:T, 0:T])
        g_e = pp.tile([1, 6 * T], F32)
        S.activation(g_e[:], gats_p[:].rearrange("o g t -> o (g t)"), AF.Exp,
                     scale=-1.0)
        g_w = pp.tile([1, 6 * T], F32)
        V.tensor_scalar(g_w[:], g_e[:], 1.0, None, op0=OP.add)
        V.reciprocal(gr[:].rearrange("o g t -> o (g t)"), g_w[:])
        ag_t = gr[0:1, 4, :]
        wg_t = gr[0:1, 5, :]
        V.tensor_tensor(c1p[:], ag_t, wg_t, op=OP.mult)
        V.tensor_scalar(cn1[:], c1p[:], -1.0, None, op0=OP.mult)
        V.tensor_tensor(c2[:], wg_t, c1p[:], op=OP.subtract)

        # modes softmax -> rows per t
        me = pp.tile([T, 12], F32)
        S.activation(me[:], iface[:, O_MD:O_MD + 12], AF.Exp)
        me3 = me[:].rearrange("t (r m) -> t r m", m=3)
        msum = pp.tile([T, R], F32)
        V.tensor_tensor(msum[:], me3[:, :, 0], me3[:, :, 1], op=OP.add)
        V.tensor_tensor(msum[:], msum[:], me3[:, :, 2], op=OP.add)
        mrcp = pp.tile([T, R], F32)
        V.reciprocal(mrcp[:], msum[:])
        mn = pp.tile([T, 12], F32)
        mn3 = mn[:].rearrange("t (m r) -> t m r", r=R)
        me3b = me[:].rearrange("t (r m) -> t m r", m=3)
        for m in range(3):
            V.tensor_tensor(mn3[:, m, :], me3b[:, m, :], mrcp[:], op=OP.mult)
        # three m-blocks at base partition 0: modes0/1/2 [4, T]
        mblk_p = psS.tile([R, 3, T], F32, tag="s")
        for m in range(3):
            P.transpose(mblk_p[:, m, :], mn[:, 4 * m:4 * (m + 1)],
                        ident[0:T, 0:T])
        mblk = pp.tile([R, 3, T], F32)
        V.tensor_copy(mblk[:], mblk_p[:])
        m1sel_p = psS.tile([1, R, T], F32, tag="s")
        for r in range(R):
            P.matmul(m1sel_p[0:1, r, :], ident[0:R, r:r + 1], mblk[:, 1, :])
        V.tensor_copy(modes1[:].rearrange("o r t -> o (r t)"),
                      m1sel_p[:].rearrange("o r t -> o (r t)"))
        # flatten rows r of m-block 0/2 onto partition 0 via selector matmuls
        mrows_p = psS.tile([1, 2, R, T], F32, tag="s")
        for r in range(R):
            P.matmul(mrows_p[0:1, 0, r, :], ident[0:R, r:r + 1], mblk[:, 0, :])
            P.matmul(mrows_p[0:1, 1, r, :], ident[0:R, r:r + 1], mblk[:, 2, :])
        mrows = pp.tile([1, 2, R, T], F32)
        V.tensor_copy(mrows[:].rearrange("o a r t -> o (a r t)"),
                      mrows_p[:].rearrange("o a r t -> o (a r t)"))
        mb0_p = psB.tile([128, R * T], F32, tag="wb")
        P.matmul(mb0_p[:], ones[0:1, :], mrows[0:1, 0, :, :])
        V.tensor_copy(mbs0[:].rearrange("p r t -> p (r t)"), mb0_p[:])
        mb2_p = psB.tile([128, R * T], F32, tag="pb", bufs=2)
        P.matmul(mb2_p[:], ones[0:1, :], mrows[0:1, 1, :, :])
        V.tensor_copy(mbs2[:].rearrange("p r t -> p (r t)"), mb2_p[:])

    # ---------------- initial state ----------------
    mem_nrm = sp.tile([128, NT, W], F32, tag="mem_nrm")
    for c in range(NT):
        DMA.dma_start(mem_nrm[:, c, :],
                      m0_d[128 * c:128 * (c + 1), :])
    memT_p = psB.tile([W, N], F32, tag="wb")
    for c in range(NT):
        P.transpose(memT_p[:, 128 * c:128 * (c + 1)],
                    mem_nrm[:, c, :], ident[:])
    memT = sp.tile([W, N], F32, tag="memT")
    V.tensor_copy(memT[:], memT_p[:])

    # initial norm: PM-layout sqn -> msq -> Ln/Exp
    sqn0 = wp.tile([128, NT, W], F32, tag="sqn")
    G_.tensor_tensor(sqn0[:], mem_nrm[:], mem_nrm[:], op=OP.mult)
    msq0 = wp.tile([128, NT], F32, tag="msq")
    V.tensor_reduce(msq0[:], sqn0[:], axis=mybir.AxisListType.X, op=OP.add)
    lms0 = wp.tile([128, NT], F32, tag="lms")
    S.activation(lms0[:], msq0[:], AF.Ln)
    mnorm_i = sp.tile([128, NT], F32, tag="mnorm")
    S.activation(mnorm_i[:], lms0[:], AF.Exp, scale=-0.5)

    L = sp.tile([128, NT, N], BF16, tag="L")
    G_.memset(L[:], 0.0)
    LT0 = sp.tile([128, NT, N], BF16, tag="LT")
    G_.memset(LT0[:], 0.0)
    dL0 = sp.tile([128, NT], F32, tag="dL")
    G_.memset(dL0[:], 0.0)

    st = dict(memT=memT, mem_nrm=mem_nrm, mnorm=mnorm_i, L=L, LT=LT0,
              dL=dL0, u_pm=None, prec_pm=None, prec_fl=None,
              pbs=None, rwT=None)

    for t in range(T):
        st = _step(nc, t, st, cp, sp, wp, psB, psM, psS,
                   ident, ident_b, ones, ones_b, iface, wvR, keysc, neg_er,
                   gr, c1p, cn1, c2, modes1, mbs0, mbs2, nege0_pm, out_sb)

    DMA.dma_start(out_d[:].rearrange("t r w -> w t r"), out_sb[:])


def _step(nc, t, st, cp, sp, wp, psB, psM, psS,
          ident, ident_b, ones, ones_b, iface, wvR, keysc, neg_er, gr, c1p,
          cn1, c2, modes1, mbs0, mbs2, nege0_pm, out_sb):
    V, S, P, G_, DMA = nc.vector, nc.scalar, nc.tensor, nc.gpsimd, nc.sync
    memT, mem_nrm, mnorm = st["memT"], st["mem_nrm"], st["mnorm"]
    L, LT, dL, u_pm = st["L"], st["LT"], st["dL"], st["u_pm"]
    prec_pm, prec_fl, pbs, rwT = (st["prec_pm"], st["prec_fl"], st["pbs"],
                                  st["rwT"])
    last = (t == T - 1)

    if t == 0:
        na_pm, nsa = nege0_pm, None
    else:
        na_pm, nsa = st["na_pm"], st["nsa"]

    # ---- pbs broadcast (prev-step prec; runs at step start) ----
    if t > 0:
        pb_p = psB.tile([128, N], F32, tag="pb", bufs=2)
        P.matmul(pb_p[:], ones_b[0:1, :], prec_fl[:])
        pbs = wp.tile([128, N], BF16, tag="pbs")
        S.activation(pbs[:], pb_p[:], AF.Copy)

    # ---- write content softmax (PM) ----
    wdots_p = psS.tile([128, NT], F32, tag="s")
    for b in range(NT):
        P.matmul(wdots_p[:, b:b + 1], memT[:, 128 * b:128 * (b + 1)],
                 keysc[:, 4, t:t + 1])
    wlog = wp.tile([128, NT], F32, tag="wlog")
    V.tensor_tensor(wlog[:], wdots_p[:], mnorm[:], op=OP.mult)
    wexp_pm = wp.tile([128, NT], F32, tag="wexp")
    S.activation(wexp_pm[:], wlog[:], AF.Exp)
    wps_p = psS.tile([1, NT], F32, tag="s")
    P.matmul(wps_p[:], ones[:, 0:1], wexp_pm[:])
    wsum = wp.tile([1, 1], F32, tag="wsum")
    V.tensor_reduce(wsum[:], wps_p[:], axis=mybir.AxisListType.X, op=OP.add)
    wrs = wp.tile([1, 1], F32, tag="wrs")
    V.reciprocal(wrs[:], wsum[:])
    cw = wp.tile([1, 1], F32, tag="cw")
    V.tensor_tensor(cw[:], wrs[:], c2[0:1, t:t + 1], op=OP.mult)

    # ---- ww assembly (PM) ----
    cn1b_p = psS.tile([128, 1], F32, tag="s")
    P.matmul(cn1b_p[:], ones[0:1, :], cn1[0:1, t:t + 1])
    cwb_p = psS.tile([128, 1], F32, tag="s")
    P.matmul(cwb_p[:], ones[0:1, :], cw[:])
    wwx = wp.tile([128, NT], F32, tag="wwx")
    V.tensor_scalar(wwx[:], na_pm[:], cn1b_p[:, 0:1], None, op0=OP.mult)
    ww_pm = wp.tile([128, NT], F32, tag="wwpm")
    V.scalar_tensor_tensor(ww_pm[:], wexp_pm[:], cwb_p[:, 0:1], wwx[:],
                           op0=OP.mult, op1=OP.add)
    if t > 0:
        omw_pm = wp.tile([128, NT], F32, tag="omw")
        V.tensor_scalar(omw_pm[:], ww_pm[:], -1.0, 1.0, op0=OP.mult,
                        op1=OP.add)
    if st.get("fin") is not None:
        st["fin"]()
        st["fin"] = None

    wwpm_b = wp.tile([128, NT], BF16, tag="wwpmb")
    V.tensor_copy(wwpm_b[:], ww_pm[:])
    ww_tp = psS.tile([1, N], BF16, tag="s")
    for c in range(NT):
        P.transpose(ww_tp[0:1, 128 * c:128 * (c + 1)], wwpm_b[:, c:c + 1],
                    ident_b[:])
    ww_fl = wp.tile([1, N], BF16, tag="wwfl")
    S.copy(ww_fl[:], ww_tp[:])

    # ---- wbs broadcast (bf16, for the link ts ops) ----
    if t > 0:
        wb_p = psB.tile([128, N], F32, tag="wb")
        P.matmul(wb_p[:], ones_b[0:1, :], ww_fl[:])
        wbs = wp.tile([128, N], BF16, tag="wbs")
        S.activation(wbs[:], wb_p[:], AF.Copy)

    # ---- memory head ----
    wwb_p = psM.tile([W, N], F32, tag="wwb")
    P.matmul(wwb_p[:], ones_b[0:1, 0:W], ww_fl[:])
    add_p = psM.tile([W, N], F32, tag="add")
    P.matmul(add_p[:], wvR[0:1, t, :], ww_fl[:])
    keep = wp.tile([W, N], F32, tag="keep")
    V.tensor_scalar(keep[:], wwb_p[:], neg_er[:, t:t + 1], 1.0,
                    op0=OP.mult, op1=OP.add)
    m1 = wp.tile([W, N], F32, tag="m1")
    G_.tensor_tensor(m1[:], memT[:], keep[:], op=OP.mult)

    # ---- usage update ----
    if last:
        u_pm_n = u_pm
    else:
        u_pm_n = sp.tile([128, NT], F32, tag="u_pm")
        if t == 0:
            V.tensor_copy(u_pm_n[:], ww_pm[:])
        else:
            fgb_p = psS.tile([128, R], F32, tag="s")
            P.matmul(fgb_p[:], ones[0:1, :], gr[0:1, 0:R, t])
            yyT = wp.tile([128, NT, R], F32, tag="yyT")
            V.scalar_tensor_tensor(
                yyT[:], fgb_p[:, None, :].broadcast_to([128, NT, R]), -1.0,
                rwT[:].rearrange("p (c r) -> p c r", r=R),
                op0=OP.mult, op1=OP.mult)
            om = wp.tile([128, NT, R], F32, tag="om")
            V.tensor_scalar(om[:], yyT[:], 1.0, None, op0=OP.add)
            p1u = wp.tile([128, NT], F32, tag="p1u")
            G_.tensor_tensor(p1u[:], om[:, :, 0], om[:, :, 1], op=OP.mult)
            p2u = wp.tile([128, NT], F32, tag="p2u")
            G_.tensor_tensor(p2u[:], om[:, :, 2], om[:, :, 3], op=OP.mult)
            psi = wp.tile([128, NT], F32, tag="psi")
            G_.tensor_tensor(psi[:], p1u[:], p2u[:], op=OP.mult)
            omu = wp.tile([128, NT], F32, tag="omu")
            V.tensor_scalar(omu[:], u_pm[:], -1.0, 1.0, op0=OP.mult,
                            op1=OP.add)
            tn = wp.tile([128, NT], F32, tag="tn")
            V.scalar_tensor_tensor(tn[:], ww_pm[:], 1.0, omu[:],
                                   op0=OP.subtract, op1=OP.mult)
            V.scalar_tensor_tensor(u_pm_n[:], tn[:], 1.0, psi[:],
                                   op0=OP.add, op1=OP.mult)

    # ---- allocation compare inputs (flat u + broadcast; exact fp32) ----
    if not last:
        u_tp = psS.tile([1, N], F32, tag="s")
        for c in range(NT):
            P.transpose(u_tp[0:1, 128 * c:128 * (c + 1)], u_pm_n[:, c:c + 1],
                        ident[:])
        u_fl_n = wp.tile([1, N], F32, tag="ufl")
        S.copy(u_fl_n[:], u_tp[:])
        ub_p = psM.tile([128, N], F32, tag="wwb")
        P.matmul(ub_p[:], ones[0:1, :], u_fl_n[:])
        ubs = wp.tile([128, N], F32, tag="ubs")
        S.copy(ubs[:], ub_p[:])
        ucl = wp.tile([128, NT], F32, tag="ucl")
        V.tensor_scalar(ucl[:], u_pm_n[:], 1e-38, None, op0=OP.max)
        lnu = wp.tile([128, NT], F32, tag="lnu")
        S.activation(lnu[:], ucl[:], AF.Ln)

    # ---- prec update ----
    if not last:
        prec_pm_n = sp.tile([128, NT], F32, tag="prec_pm")
        if t == 0:
            V.tensor_copy(prec_pm_n[:], ww_pm[:])
        else:
            swa = wp.tile([1, 1], F32, tag="swa")
            G_.tensor_tensor(swa[:], nsa[:], cn1[0:1, t:t + 1], op=OP.mult)
            sw = wp.tile([1, 1], F32, tag="sw")
            G_.tensor_tensor(sw[:], swa[:], c2[0:1, t:t + 1], op=OP.add)
            omsw = wp.tile([1, 1], F32, tag="omsw")
            V.tensor_scalar(omsw[:], sw[:], -1.0, 1.0, op0=OP.mult,
                            op1=OP.add)
            omsw_p = psS.tile([128, 1], F32, tag="s")
            P.matmul(omsw_p[:], ones[0:1, :], omsw[:])
            V.scalar_tensor_tensor(prec_pm_n[:], prec_pm[:], omsw_p[:, 0:1],
                                   ww_pm[:], op0=OP.mult, op1=OP.add)
        p_tp = psS.tile([1, N], F32, tag="s")
        for c in range(NT):
            P.transpose(p_tp[0:1, 128 * c:128 * (c + 1)], prec_pm_n[:, c:c + 1],
                        ident[:])
        prec_fl_n = sp.tile([1, N], BF16, tag="prec_fl")
        S.copy(prec_fl_n[:], p_tp[:])
    else:
        prec_pm_n, prec_fl_n = prec_pm, prec_fl

    # ---- mode-scaled read weights + link diagonal tracker ----

    # ---- link loop with interleaved memT_n / Gt compares ----
    comb_eng = [(G_, G_), (G_, G_), (G_, V), (G_, V)]
    if t == 0:
        L_n, LT_n = L, LT
        memT_n = sp.tile([W, N], F32, tag="memT")
        V.tensor_tensor(memT_n[:], m1[:], add_p[:], op=OP.add)
        if not last:
            Gt_n = wp.tile([128, NT, N], F32, tag="G", bufs=1)
            for c in range(NT):
                V.tensor_scalar(Gt_n[:, c, :], ubs[:], u_pm_n[:, c:c + 1],
                                None, op0=OP.is_gt)
    else:
        L_n = sp.tile([128, NT, N], BF16, tag="L")
        LT_n = sp.tile([128, NT, N], BF16, tag="LT")
        memT_n = sp.tile([W, N], F32, tag="memT")
        if not last:
            Gt_n = wp.tile([128, NT, N], F32, tag="G", bufs=1)
        for c in range(NT):
            w1 = wp.tile([128, N], BF16, tag=f"w1_{c % 2}")
            V.tensor_scalar(w1[:], wbs[:], omw_pm[:, c:c + 1], None,
                            op0=OP.subtract)
            p1 = wp.tile([128, N], BF16, tag=f"p1_{c % 2}")
            V.tensor_scalar(p1[:], pbs[:], ww_pm[:, c:c + 1], None,
                            op0=OP.mult)
            p1T = wp.tile([128, N], BF16, tag=f"p1T_{c % 2}")
            V.tensor_scalar(p1T[:], wbs[:], prec_pm[:, c:c + 1], None,
                            op0=OP.mult)
            t1 = wp.tile([128, N], BF16, tag=f"t1_{c % 2}")
            G_.tensor_tensor(t1[:], w1[:], L[:, c, :], op=OP.mult)
            t1T = wp.tile([128, N], BF16, tag=f"t1T_{c % 2}")
            G_.tensor_tensor(t1T[:], w1[:], LT[:, c, :], op=OP.mult)
            eL, eLT = comb_eng[c]
            eL.tensor_tensor(L_n[:, c, :], p1[:], t1[:], op=OP.subtract)
            eLT.tensor_tensor(LT_n[:, c, :], p1T[:], t1T[:], op=OP.subtract)
            if c > 0 and not last:
                cc = c - 1
                V.tensor_scalar(Gt_n[:, cc, :], ubs[:],
                                u_pm_n[:, cc:cc + 1], None, op0=OP.is_gt)
        V.tensor_tensor(memT_n[:], m1[:], add_p[:], op=OP.add)
        if not last:
            V.tensor_scalar(Gt_n[:, 3, :], ubs[:], u_pm_n[:, 3:4],
                            None, op0=OP.is_gt)

    if t > 0:
        wp_pm = wp.tile([128, NT], F32, tag="wppm")
        G_.tensor_tensor(wp_pm[:], ww_pm[:], prec_pm[:], op=OP.mult)
        dmul = wp.tile([128, NT], F32, tag="dmul")
        V.tensor_scalar(dmul[:], ww_pm[:], -2.0, 1.0, op0=OP.mult, op1=OP.add)
        dL_n = sp.tile([128, NT], F32, tag="dL")
        V.scalar_tensor_tensor(dL_n[:], dL[:], 1.0, dmul[:],
                               op0=OP.mult, op1=OP.mult)
        G_.tensor_tensor(dL_n[:], dL_n[:], wp_pm[:], op=OP.add)
    else:
        dL_n = dL
    if t > 0:
        rwTm0 = wp.tile([128, NT * R], BF16, tag="rwTm0")
        V.tensor_tensor(rwTm0[:].rearrange("p (c r) -> p c r", r=R),
                        rwT[:].rearrange("p (c r) -> p c r", r=R),
                        mbs0[:, None, :, t].broadcast_to([128, NT, R]),
                        op=OP.mult)
        rwTm2 = wp.tile([128, NT * R], BF16, tag="rwTm2")
        V.tensor_tensor(rwTm2[:].rearrange("p (c r) -> p c r", r=R),
                        rwT[:].rearrange("p (c r) -> p c r", r=R),
                        mbs2[:, None, :, t].broadcast_to([128, NT, R]),
                        op=OP.mult)

    # ---- memory norm chain ----
    mem_nrm_p = psS.tile([128, NT, W], F32, tag="mn", bufs=1)
    for c in range(NT):
        P.transpose(mem_nrm_p[:, c, :], memT_n[:, 128 * c:128 * (c + 1)],
                    ident[0:W, 0:W])
    sqn = wp.tile([128, NT, W], F32, tag="sqn")
    S.activation(sqn[:], mem_nrm_p[:], AF.Square)
    msq = wp.tile([128, NT], F32, tag="msq")
    V.tensor_reduce(msq[:], sqn[:], axis=mybir.AxisListType.X, op=OP.add)
    mem_nrm_n = sp.tile([128, NT, W], F32, tag="mem_nrm")
    S.copy(mem_nrm_n[:], mem_nrm_p[:])
    lms = wp.tile([128, NT], F32, tag="lms")
    S.activation(lms[:], msq[:], AF.Ln)
    mnorm_n = sp.tile([128, NT], F32, tag="mnorm")
    S.activation(mnorm_n[:], lms[:], AF.Exp, scale=-0.5)

    # ---- allocation log-sum ----
    if last:
        na_pm_n, nsa_n = None, None
    else:
        sT_p = psS.tile([128, NT], F32, tag="s")
        for b in range(NT):
            for c in range(NT):
                P.matmul(sT_p[:, b:b + 1], Gt_n[:, c, 128 * b:128 * (b + 1)],
                         lnu[:, c:c + 1], start=(c == 0),
                         stop=(c == NT - 1))
        es_pm = wp.tile([128, NT], F32, tag="espm")
        S.activation(es_pm[:], sT_p[:], AF.Exp)
        na_pm_n = wp.tile([128, NT], F32, tag="napm")
        if t < T - 2:
            nap = wp.tile([128, 1], F32, tag="nap")
            V.scalar_tensor_tensor(na_pm_n[:], u_pm_n[:], 1.0, es_pm[:],
                                   op0=OP.subtract, op1=OP.mult,
                                   accum_out=nap[:])
        else:
            V.scalar_tensor_tensor(na_pm_n[:], u_pm_n[:], 1.0, es_pm[:],
                                   op0=OP.subtract, op1=OP.mult)
        if t < T - 2:
            nsa_p = psS.tile([1, 1], F32, tag="s")
            P.matmul(nsa_p[:], nap[:], ones[:, 0:1])
            nsa_n = wp.tile([1, 1], F32, tag="nsa")
            V.tensor_copy(nsa_n[:], nsa_p[:])
        else:
            nsa_n = None

    # ---- read content (PM) ----
    rdots_p = psS.tile([128, NT * R], F32, tag="s")
    for b in range(NT):
        P.matmul(rdots_p[:, R * b:R * (b + 1)],
                 memT_n[:, 128 * b:128 * (b + 1)], keysc[:, 0:4, t])
    rlog = wp.tile([128, NT, R], F32, tag="rlog")
    V.tensor_tensor(rlog[:],
                    rdots_p[:].rearrange("p (c r) -> p c r", r=R),
                    mnorm_n[:, :, None].broadcast_to([128, NT, R]),
                    op=OP.mult)
    rexp_pm = wp.tile([128, NT * R], F32, tag="rexp")
    S.activation(rexp_pm[:], rlog[:].rearrange("p c r -> p (c r)"), AF.Exp)
    rps_p = psS.tile([1, NT * R], F32, tag="s")
    P.matmul(rps_p[:], ones[:, 0:1], rexp_pm[:])

    def _readmix(t, rps_p=rps_p, rexp_pm=rexp_pm):
        rsum = wp.tile([1, R], F32, tag="rsum")
        V.tensor_reduce(rsum[:], rps_p[:].rearrange("o (c r) -> o r c", r=R),
                        axis=mybir.AxisListType.X, op=OP.add)
        rsr = wp.tile([1, R], F32, tag="rsr")
        V.reciprocal(rsr[:], rsum[:])
        s1c = wp.tile([1, R], F32, tag="s1c")
        V.tensor_tensor(s1c[:], rsr[:], modes1[0:1, :, t], op=OP.mult)
        s1cb_p = psS.tile([128, R], F32, tag="s")
        P.matmul(s1cb_p[:], ones[0:1, :], s1c[:])
        cnt = wp.tile([128, NT, R], F32, tag="cnt")
        V.tensor_tensor(cnt[:], rexp_pm[:].rearrange("p (c r) -> p c r", r=R),
                        s1cb_p[:, None, :].broadcast_to([128, NT, R]),
                        op=OP.mult)
        return cnt

    rwT_n = sp.tile([128, NT * R], F32, tag="rwT")
    if t > 0:
        def _fin(rwT_n=rwT_n, mem_nrm_n=mem_nrm_n, dL_n=dL_n,
                 L_n=L_n, LT_n=LT_n, rwTm0=rwTm0, rwTm2=rwTm2, t=t):
            cnt = _readmix(t)
            s02 = wp.tile([128, NT, R], F32, tag="s02")
            G_.tensor_tensor(s02[:],
                             rwTm0[:].rearrange("p (c r) -> p c r", r=R),
                             rwTm2[:].rearrange("p (c r) -> p c r", r=R),
                             op=OP.add)
            corr = wp.tile([128, NT, R], F32, tag="corr")
            V.tensor_tensor(corr[:], s02[:],
                            dL_n[:, :, None].broadcast_to([128, NT, R]),
                            op=OP.mult)
            cnt2 = wp.tile([128, NT, R], F32, tag="cnt2")
            G_.tensor_tensor(cnt2[:], cnt[:], corr[:], op=OP.subtract)
            rwT_p = psS.tile([128, NT * R], F32, tag="s")
            for b in range(NT):
                blk = slice(128 * b, 128 * (b + 1))
                for c in range(NT):
                    P.matmul(rwT_p[:, R * b:R * (b + 1)], L_n[:, c, blk],
                             rwTm0[:, R * c:R * (c + 1)],
                             start=(c == 0), stop=False)
                for c in range(NT):
                    P.matmul(rwT_p[:, R * b:R * (b + 1)], LT_n[:, c, blk],
                             rwTm2[:, R * c:R * (c + 1)],
                             start=False, stop=(c == NT - 1))
            V.tensor_tensor(rwT_n[:], cnt2[:].rearrange("p c r -> p (c r)"),
                            rwT_p[:], op=OP.add)
            rwd_p = psS.tile([W, R], F32, tag="s")
            for c in range(NT):
                P.matmul(rwd_p[:], mem_nrm_n[:, c, :],
                         rwT_n[:, R * c:R * (c + 1)],
                         start=(c == 0), stop=(c == NT - 1))
            S.copy(out_sb[:, t, :], rwd_p[:])
    else:
        def _fin(rwT_n=rwT_n, mem_nrm_n=mem_nrm_n, t=t):
            cnt = _readmix(t)
            V.tensor_copy(rwT_n[:], cnt[:].rearrange("p c r -> p (c r)"))
            rwd_p = psS.tile([W, R], F32, tag="s")
            for c in range(NT):
                P.matmul(rwd_p[:], mem_nrm_n[:, c, :],
                         rwT_n[:, R * c:R * (c + 1)],
                         start=(c == 0), stop=(c == NT - 1))
            S.copy(out_sb[:, t, :], rwd_p[:])
    if last:
        _fin()
    else:
        st_fin = _fin

    return dict(memT=memT_n, mem_nrm=mem_nrm_n, mnorm=mnorm_n, L=L_n,
                LT=LT_n, dL=dL_n, u_pm=u_pm_n, na_pm=na_pm_n,
                nsa=nsa_n, prec_pm=prec_pm_n, prec_fl=prec_fl_n, pbs=pbs,
                rwT=rwT_n, fin=(None if last else st_fin))


# ---------------------------------------------------------------------------
_NC_CACHE = {}


def _get_nc():
    if "nc" not in _NC_CACHE:
        _NC_CACHE["nc"] = build_nc()
    return _NC_CACHE["nc"]


def _consts():
    ident = np.eye(128, dtype=np.float32)
    return (ident,)


def make_in_maps(controller_output, W_if, b_if, memory0):
    (ident,) = _consts()
    maps = []
    for b in range(B):
        maps.append({
            "co": np.ascontiguousarray(controller_output[b]),
            "wif": np.ascontiguousarray(W_if),
            "bif": np.ascontiguousarray(b_if.reshape(1, IF)),
            "mem0": np.ascontiguousarray(memory0[b]),
            "ident": ident,
        })
    return maps


def kernel(controller_output, W_if, b_if, memory0):
    from concourse.bass_utils import run_bass_kernel_spmd
    controller_output = np.asarray(controller_output, dtype=np.float32)
    W_if = np.asarray(W_if, dtype=np.float32)
    b_if = np.asarray(b_if, dtype=np.float32)
    memory0 = np.asarray(memory0, dtype=np.float32)
    nc = _get_nc()
    maps = make_in_maps(controller_output, W_if, b_if, memory0)
    # Retry once on non-finite output: a stale device (e.g. after an
    # earlier aborted run) can poison cores on the first dispatch.
    for _ in range(2):
        res = run_bass_kernel_spmd(nc, maps, core_ids=list(range(B)))
        out = np.stack([res.results[b]["out"] for b in range(B)], axis=0)
        if np.isfinite(out).all():
            break
    return out


if __name__ == "__main__":
    mode = sys.argv[1] if len(sys.argv) > 1 else "sim"
    sys.path.insert(0, "/root/problem")
    import jax
    with jax.default_device(jax.devices("cpu")[0]):
        import reference
        inputs = {k: np.asarray(v) for k, v in reference.setup_inputs().items()}
        expected = np.asarray(reference.reference(**inputs))

    if mode == "sim":
        from concourse.bass_interp import CoreSim
        nc = build_nc()
        maps = make_in_maps(inputs["controller_output"], inputs["W_if"],
                            inputs["b_if"], inputs["memory0"])
        sim = CoreSim(nc)
        for k, v in maps[0].items():
            sim.tensor(k)[:] = v
        sim.simulate()
        got = sim.tensor("out").copy()
        exp = expected[0]
        err = np.abs(got - exp)
        rel = np.linalg.norm(got - exp) / (np.linalg.norm(exp) + 1e-12)
        print("sim modeled time (ns):", sim.time)
        print("max abs err:", err.max(), " rel err:", rel)
    else:
        got = kernel(**inputs)
        rel = np.linalg.norm(got - expected) / (np.linalg.norm(expected) + 1e-12)
        print("max abs err:", np.abs(got - expected).max(), " rel err:", rel)
